# revision 1
# baseline (speedup 1.0000x reference)
"""Trainium2 Bass kernel for nn_ReachabilityClassifierTransformer.

Data-parallel over batch: 16 samples / 8 cores = 2 samples per core.
Each core runs the full network (6-layer encoder + 4-layer decoder + head)
on its 2 samples. No collectives.

Device layout conventions (per core):
  - Activations are kept FEATURE-MAJOR in SBUF: tile [128, KC, T] holds
    X.T, i.e. element [p, k, t] = X[t, k*128+p]. T = 2*512 tokens
    (sample-major concat).
  - All weights are pre-transposed on host to [in_feat, out_feat] and laid
    out as [128, KC_in, O] (partition = in-feature % 128).
  - matmul(out_psum[M,N], lhsT=[K,M], rhs=[K,N]) computes lhsT.T @ rhs with
    K on partitions.  "Option B": lhsT = weight chunk -> output feature-major.
    "Option A": lhsT = activation chunk -> output token-major (used for V).
  - float32r is used for every matmul operand (full-rate fp32 on the PE).
  - Encoder stage-1 q,k output features are de-interleaved (even feats then
    odd feats) via host-side column permutation of in_proj, so RoPE becomes
    contiguous block ops; the roped result is in natural order again.
  - Softmax: scores are computed transposed (S.T = K_h @ Q_h.T per 128-row
    chunk), exp'd without max subtraction (|scores/8| < 1 for this model),
    and the denominator comes free from a ones-column appended to V.
"""
import numpy as np

import concourse.bass as bass
import concourse.mybir as mybir
import concourse.tile as tile
from concourse import bacc
from concourse.bass_utils import run_bass_kernel_spmd

AF = mybir.ActivationFunctionType
ALU = mybir.AluOpType
F32 = mybir.dt.float32
F32R = mybir.dt.float32r
F16 = mybir.dt.float16

B, S, D, FF, H, LE, LD, M = 16, 512, 512, 2048, 8, 6, 4, 2048
ROPE_BASE = 10000.0
LN_EPS = 1e-5
NCORES = 8
BL = B // NCORES          # 2 samples per core
T = BL * S                # 1024 tokens per core
KC = D // 128             # 4 feature chunks
FC = FF // 128            # 16
MC = M // 128             # 16
DH = D // H               # 64


# ----------------------------------------------------------------------------
# host-side helpers
# ----------------------------------------------------------------------------

def _chunked(wT):
    """[Din, O] -> [128, Din//128, O] contiguous."""
    Din, O = wT.shape
    return np.ascontiguousarray(
        wT.reshape(Din // 128, 128, O).transpose(1, 0, 2)).astype(np.float32)


def _bias_cols(b):
    """[O] -> [128, O//128]  (column per 128-chunk)."""
    O = b.shape[0]
    return np.ascontiguousarray(b.reshape(O // 128, 128).T).astype(np.float32)


_DEINT = np.concatenate([np.arange(0, D, 2), np.arange(1, D, 2)])  # de-interleave


def prep_weights(inp, le=LE, ld=LD):
    """Host-side weight prep -> dict of arrays shared by all cores."""
    out = {}
    g = {k: np.asarray(v, np.float32) for k, v in inp.items()}

    out["mpwT"] = np.ascontiguousarray(g["morph_proj_w"].T)        # [3, 512]
    out["mpb"] = _bias_cols(g["morph_proj_b"])                     # [128, 4]
    out["ppwT"] = np.ascontiguousarray(g["pose_proj_w"].T)         # [9, 512]
    out["ppb"] = _bias_cols(g["pose_proj_b"])

    # rope grids, de-interleaved frequency order: [128, 2, 512]
    freq = 1.0 / ROPE_BASE ** (np.arange(0, D, 2, dtype=np.float64) / D)
    ang = np.outer(np.arange(S, dtype=np.float64), freq)           # [512, 256]
    out["gridc"] = _chunked(np.cos(ang).T.astype(np.float32).reshape(256, S)).astype(np.float16)
    out["grids"] = _chunked(np.sin(ang).T.astype(np.float32).reshape(256, S)).astype(np.float16)

    e_w1, e_w1b, e_w2, e_w2b, e_vb = [], [], [], [], []
    e_ow, e_owb, e_l1, e_l1b, e_l2, e_l2b = [], [], [], [], [], []
    for i in range(le):
        w1 = g["enc_in_w"][i] * g["enc_n1_g"][i][None, :]          # fold n1 g
        b1 = g["enc_in_b"][i] + g["enc_in_w"][i] @ g["enc_n1_b"][i]
        # stage-1: de-interleave q,k output columns
        perm = np.concatenate([_DEINT, D + _DEINT, 2 * D + np.arange(D)])
        e_w1.append(_chunked(np.ascontiguousarray(w1[perm].T)))    # [128,4,1536]
        e_w1b.append(_bias_cols(b1[perm]))                         # [128,12]
        # stage-2 (natural order, raw weights - the faithful quirk)
        w2 = g["enc_in_w"][i][: 2 * D]                             # Wq;Wk
        e_w2.append(_chunked(np.ascontiguousarray(w2.T)))          # [128,4,1024]
        e_w2b.append(_bias_cols(g["enc_in_b"][i][: 2 * D]))        # [128,8]
        e_vb.append(g["enc_in_b"][i][2 * D:][None, :])             # [1,512]
        e_ow.append(_chunked(np.ascontiguousarray(g["enc_out_w"][i].T)))
        e_owb.append(_bias_cols(g["enc_out_b"][i]))
        l1 = g["enc_l1_w"][i] * g["enc_n2_g"][i][None, :]
        l1b = g["enc_l1_b"][i] + g["enc_l1_w"][i] @ g["enc_n2_b"][i]
        e_l1.append(_chunked(np.ascontiguousarray(l1.T)))          # [128,4,2048]
        e_l1b.append(_bias_cols(l1b))                              # [128,16]
        e_l2.append(_chunked(np.ascontiguousarray(g["enc_l2_w"][i].T)))
        e_l2b.append(_bias_cols(g["enc_l2_b"][i]))                 # [128,4]
    out["ew1T"], out["ew1b"] = np.stack(e_w1), np.stack(e_w1b)
    out["ew2T"], out["ew2b"] = np.stack(e_w2), np.stack(e_w2b)
    out["evb"] = np.stack(e_vb)
    out["eowT"], out["eowb"] = np.stack(e_ow), np.stack(e_owb)
    out["el1T"], out["el1b"] = np.stack(e_l1), np.stack(e_l1b)
    out["el2T"], out["el2b"] = np.stack(e_l2), np.stack(e_l2b)

    d_in, d_inb, d_vb, d_ow, d_owb = [], [], [], [], []
    d_m1, d_m1b, d_m2, d_m2b = [], [], [], []
    for i in range(ld):
        w = g["dec_in_w"][i].copy()
        b = g["dec_in_b"][i].copy()
        w[:D] = w[:D] * g["dec_n1_g"][i][None, :]                  # Wq <- dec_n1
        b[:D] = b[:D] + g["dec_in_w"][i][:D] @ g["dec_n1_b"][i]
        w[D:] = w[D:] * g["enc_final_g"][None, :]                  # Wk,Wv <- enc_final
        b[D:] = b[D:] + g["dec_in_w"][i][D:] @ g["enc_final_b"]
        d_in.append(_chunked(np.ascontiguousarray(w.T)))           # [128,4,1536]
        d_inb.append(_bias_cols(b))
        d_vb.append(b[2 * D:][None, :])                            # [1,512]
        d_ow.append(_chunked(np.ascontiguousarray(g["dec_out_w"][i].T)))
        d_owb.append(_bias_cols(g["dec_out_b"][i]))
        m1 = g["dec_m1_w"][i] * g["dec_n2_g"][i][None, :]
        m1b = g["dec_m1_b"][i] + g["dec_m1_w"][i] @ g["dec_n2_b"][i]
        d_m1.append(_chunked(np.ascontiguousarray(m1.T)))          # [128,4,2048]
        d_m1b.append(_bias_cols(m1b))
        d_m2.append(_chunked(np.ascontiguousarray(g["dec_m2_w"][i].T)))
        d_m2b.append(_bias_cols(g["dec_m2_b"][i]))
    out["dinT"], out["dinb"] = np.stack(d_in), np.stack(d_inb)
    out["dvb"] = np.stack(d_vb)
    out["dowT"], out["dowb"] = np.stack(d_ow), np.stack(d_owb)
    out["dm1T"], out["dm1b"] = np.stack(d_m1), np.stack(d_m1b)
    out["dm2T"], out["dm2b"] = np.stack(d_m2), np.stack(d_m2b)

    hw = (g["head_w"] * g["head_g"][None, :])[0]                   # [512]
    out["hwT"] = _bias_cols(hw)                                    # [128, 4]
    out["hb"] = (g["head_bias"] + g["head_w"] @ g["head_b"]).reshape(1, 1)
    return out


# ----------------------------------------------------------------------------
# device program
# ----------------------------------------------------------------------------

def build(le=LE, ld=LD):
    nc = bacc.Bacc(None, target_bir_lowering=False)

    dram = {}

    def din(name, shape, dt=F32R):
        dram[name] = nc.dram_tensor(name, list(shape), dt, kind="ExternalInput")
        return dram[name]

    # shared weights
    din("mpwT", [3, 512]); din("mpb", [128, 4], F32)
    din("ppwT", [9, 512]); din("ppb", [128, 4], F32)
    din("gridc", [128, 2, S], F16); din("grids", [128, 2, S], F16)
    din("ew1T", [le, 128, KC, 3 * D]); din("ew1b", [le, 128, 12], F32)
    din("ew2T", [le, 128, KC, 2 * D]); din("ew2b", [le, 128, 8], F32)
    din("evb", [le, 1, D])
    din("eowT", [le, 128, KC, D]); din("eowb", [le, 128, 4], F32)
    din("el1T", [le, 128, KC, FF]); din("el1b", [le, 128, 16], F32)
    din("el2T", [le, 128, FC, D]); din("el2b", [le, 128, 4], F32)
    din("dinT", [ld, 128, KC, 3 * D]); din("dinb", [ld, 128, 12], F32)
    din("dvb", [ld, 1, D])
    din("dowT", [ld, 128, KC, D]); din("dowb", [ld, 128, 4], F32)
    din("dm1T", [ld, 128, KC, M]); din("dm1b", [ld, 128, 16], F32)
    din("dm2T", [ld, 128, MC, D]); din("dm2b", [ld, 128, 4], F32)
    din("hwT", [128, KC]); din("hb", [1, 1], F32)
    # per-core inputs
    din("morphT", [3, T])
    din("poseT", [9, BL])
    y = nc.dram_tensor("y", [1, BL], F32, kind="ExternalOutput")

    with tile.TileContext(nc) as tc:
        _build_body(nc, tc, dram, y, le, ld)
    nc.compile()
    return nc


def _build_body(nc, tc, dram, y_dram, le, ld):
    import contextlib
    ctx = contextlib.ExitStack()
    with ctx:
        ctx.enter_context(nc.allow_low_precision(
            reason="float32r rounding of matmul operands is intentional"))
        persist = ctx.enter_context(tc.tile_pool(name="persist", bufs=1))
        wpool = ctx.enter_context(tc.tile_pool(name="wpool", bufs=2))
        w2pool = ctx.enter_context(tc.tile_pool(name="w2pool", bufs=1))
        owpool = ctx.enter_context(tc.tile_pool(name="owpool", bufs=1))
        bpool = ctx.enter_context(tc.tile_pool(name="bpool", bufs=2))
        a4 = ctx.enter_context(tc.tile_pool(name="a4", bufs=3))
        a8 = ctx.enter_context(tc.tile_pool(name="a8", bufs=2))
        vp = ctx.enter_context(tc.tile_pool(name="vp", bufs=1))
        rtp = ctx.enter_context(tc.tile_pool(name="rtp", bufs=3))
        vbp = ctx.enter_context(tc.tile_pool(name="vbp", bufs=1))
        scr = ctx.enter_context(tc.tile_pool(name="scr", bufs=2))
        scrrc = ctx.enter_context(tc.tile_pool(name="scrrc", bufs=1))
        smalls = ctx.enter_context(tc.tile_pool(name="smalls", bufs=2))
        b1 = ctx.enter_context(tc.tile_pool(name="b1", bufs=4, space="PSUM"))
        b2 = ctx.enter_context(tc.tile_pool(name="b2", bufs=2, space="PSUM"))
        qk2p, atp = a8, a4  # share slots/tags

        # ---------------- persistent tiles ----------------
        x = persist.tile([128, KC, T], F32R)          # residual stream (X.T)
        gridc = persist.tile([128, 2, S], F16)
        grids = persist.tile([128, 2, S], F16)
        ones128 = persist.tile([128, 1], F32R)
        ones_row = persist.tile([1, 128], F32R)
        eps_t = persist.tile([1, 1], F32)
        p = persist.tile([128, KC, BL], F32R)         # decoder latent p.T
        nc.sync.dma_start(gridc[:], dram["gridc"][:])
        nc.sync.dma_start(grids[:], dram["grids"][:])
        ones8 = persist.tile([128, 8], F32R)
        stage_f32 = rtp.tile([128, 128], F32, tag="rt")
        nc.vector.memset(stage_f32[:], 1.0)
        nc.vector.tensor_copy(ones128[:], stage_f32[:, 0:1])
        nc.vector.tensor_copy(ones_row[:], stage_f32[0:1, :])
        nc.vector.tensor_copy(ones8[:], stage_f32[:, 0:8])
        nc.vector.memset(eps_t[:], LN_EPS)

        def c32(ap):
            return ap.bitcast(F32)

        def ln(x_tile, sl, n_tok, h_out, out_sl):
            """h_out[:, :, out_sl] = LayerNorm_features(x_tile[:, :, sl])."""
            sq = a4.tile([128, KC, n_tok], F32R, tag="a4")
            for k in range(KC):
                nc.scalar.activation(sq[:, k, :], x_tile[:, k, sl], AF.Square)
            sum_ps = b2.tile([1, n_tok], F32, tag="b2")
            sq_ps = b2.tile([1, n_tok], F32, tag="b2")
            cv = (lambda ap: ap) if n_tok >= 256 else c32
            for k in range(KC):
                nc.tensor.matmul(sum_ps[:], cv(ones128[:]), cv(x_tile[:, k, sl]),
                                 start=(k == 0), stop=(k == KC - 1))
            for k in range(KC):
                nc.tensor.matmul(sq_ps[:], cv(ones128[:]), cv(sq[:, k, :]),
                                 start=(k == 0), stop=(k == KC - 1))
            ms = scr.tile([1, n_tok], F32, tag="scr")
            t2 = scr.tile([1, n_tok], F32, tag="scr")
            rc = scrrc.tile([1, 2, n_tok], F32R, tag="scr_rc")
            nc.scalar.activation(ms[:], sum_ps[:], AF.Copy, scale=1.0 / D)
            # t2 = E[x^2] - m^2  (stt: (ms * ms) subtracted via reverse op)
            nc.vector.tensor_tensor(t2[:], ms[:], ms[:], ALU.mult)      # m^2
            nc.vector.scalar_tensor_tensor(
                t2[:], sq_ps[:], 1.0 / D, t2[:], ALU.mult, ALU.subtract)
            # t2 = sqrt(var + eps)
            nc.scalar.activation(t2[:], t2[:], AF.Sqrt, bias=eps_t[:])
            nc.vector.reciprocal(rc[:, 0, :], t2[:])                    # r
            nc.vector.scalar_tensor_tensor(
                rc[:, 1, :], ms[:], -1.0, rc[:, 0, :], ALU.mult, ALU.mult)  # c
            r_ps = b1.tile([128, n_tok], F32, tag="b1")
            c_ps = b1.tile([128, n_tok], F32, tag="b1")
            nc.tensor.matmul(r_ps[:], cv(ones_row[:]), cv(rc[:, 0, :]),
                             start=True, stop=True)
            nc.tensor.matmul(c_ps[:], cv(ones_row[:]), cv(rc[:, 1, :]),
                             start=True, stop=True)
            for k in range(KC):
                nc.vector.tensor_tensor(h_out[:, k, out_sl], x_tile[:, k, sl],
                                        r_ps[:], ALU.mult)
                nc.vector.tensor_tensor(h_out[:, k, out_sl], h_out[:, k, out_sl],
                                        c_ps[:], ALU.add)

        def ln_stats_pair(x_tile):
            """One LN chain for both samples: returns rc [1, 2, T] f32r."""
            sq = a8.tile([128, KC, T], F32R, tag="a8")
            for k in range(KC):
                nc.scalar.activation(sq[:, k, :], x_tile[:, k, :], AF.Square)
            sum_ps = b2.tile([1, T], F32, tag="b2")
            sq_ps = b2.tile([1, T], F32, tag="b2")
            for nh in range(BL):
                nsl = slice(nh * S, (nh + 1) * S)
                for k in range(KC):
                    nc.tensor.matmul(sum_ps[:, nsl], ones128[:],
                                     x_tile[:, k, nsl],
                                     start=(k == 0), stop=(k == KC - 1))
                for k in range(KC):
                    nc.tensor.matmul(sq_ps[:, nsl], ones128[:], sq[:, k, nsl],
                                     start=(k == 0), stop=(k == KC - 1))
            rc = scrrc.tile([1, 2, T], F32R, tag="scr_rcT")
            ms = rc[:, 1, :]                      # mean parked in the c slot
            t2 = scrrc.tile([1, T], F32, tag="scrT2")
            nc.scalar.activation(ms, sum_ps[:], AF.Copy, scale=1.0 / D)
            nc.vector.tensor_tensor(t2[:], ms, ms, ALU.mult)
            nc.vector.scalar_tensor_tensor(
                t2[:], sq_ps[:], 1.0 / D, t2[:], ALU.mult, ALU.subtract)
            nc.scalar.activation(t2[:], t2[:], AF.Sqrt, bias=eps_t[:])
            nc.vector.reciprocal(rc[:, 0, :], t2[:])
            nc.vector.scalar_tensor_tensor(
                rc[:, 1, :], ms, -1.0, rc[:, 0, :], ALU.mult, ALU.mult)
            return rc

        def ln_apply(rc, x_tile, s, h_out):
            sl = slice(s * S, (s + 1) * S)
            r_ps = b1.tile([128, S], F32, tag="b1")
            c_ps = b1.tile([128, S], F32, tag="b1")
            nc.tensor.matmul(r_ps[:], ones_row[:], rc[:, 0, sl],
                             start=True, stop=True)
            nc.tensor.matmul(c_ps[:], ones_row[:], rc[:, 1, sl],
                             start=True, stop=True)
            for k in range(KC):
                nc.vector.tensor_tensor(h_out[:, k, :], x_tile[:, k, sl],
                                        r_ps[:], ALU.mult)
                nc.vector.tensor_tensor(h_out[:, k, :], h_out[:, k, :],
                                        c_ps[:], ALU.add)

        # ---------------- morph projection -> x ----------------
        morpht = a8.tile([3, T], F32R, tag="a8")
        nc.sync.dma_start(morpht[:], dram["morphT"][:])
        mpw = rtp.tile([3, 512], F32R, tag="rt")
        mpb = rtp.tile([128, 4], F32, tag="rt")
        nc.sync.dma_start(mpw[:], dram["mpwT"][:])
        nc.sync.dma_start(mpb[:], dram["mpb"][:])
        for m in range(KC):
            for s in range(BL):
                ps = b1.tile([128, S], F32, tag="b1")
                nc.tensor.matmul(ps[:], mpw[:, m * 128:(m + 1) * 128],
                                 morpht[:, s * S:(s + 1) * S], start=True, stop=True)
                nc.scalar.activation(x[:, m, s * S:(s + 1) * S], ps[:], AF.Relu,
                                     bias=mpb[:, m:m + 1])

        # ---------------- pose projection -> p ----------------
        poset = rtp.tile([9, BL], F32R, tag="rt")
        ppw = rtp.tile([9, 512], F32R, tag="rt")
        ppb = rtp.tile([128, 4], F32, tag="rt")
        nc.sync.dma_start(poset[:], dram["poseT"][:])
        nc.sync.dma_start(ppw[:], dram["ppwT"][:])
        nc.sync.dma_start(ppb[:], dram["ppb"][:])
        pps = b1.tile([128, KC, BL], F32, tag="b1")
        for m in range(KC):
            nc.tensor.matmul(pps[:, m, :], c32(ppw[:, m * 128:(m + 1) * 128]),
                             c32(poset[:]), start=True, stop=True)
        for m in range(KC):
            nc.scalar.activation(p[:, m, :], pps[:, m, :], AF.Relu,
                                 bias=ppb[:, m:m + 1])

        # ---------------- encoder layers ----------------
        for li in range(le):
            w1 = wpool.tile([128, KC, 3 * D], F32R, tag="bigw")
            nc.sync.dma_start(w1[:], dram["ew1T"][li])
            w1b = bpool.tile([128, 12], F32, tag="w1b")
            nc.sync.dma_start(w1b[:], dram["ew1b"][li])
            w2 = w2pool.tile([128, KC, 2 * D], F32R, tag="w2")
            nc.sync.dma_start(w2[:], dram["ew2T"][li])
            w2b = bpool.tile([128, 8], F32, tag="w2b")
            nc.sync.dma_start(w2b[:], dram["ew2b"][li])
            vbrow = bpool.tile([1, D], F32R, tag="vbrow")
            nc.sync.dma_start(vbrow[:], dram["evb"][li])
            ow = owpool.tile([128, KC, D], F32R, tag="ow")
            nc.sync.dma_start(ow[:], dram["eowT"][li])
            owb = bpool.tile([128, 4], F32, tag="owb")
            nc.sync.dma_start(owb[:], dram["eowb"][li])
            l1 = wpool.tile([128, KC, FF], F32R, tag="bigw")
            nc.sync.dma_start(l1[:], dram["el1T"][li])
            l1b = bpool.tile([128, 16], F32, tag="l1b")
            nc.sync.dma_start(l1b[:], dram["el1b"][li])
            l2b = bpool.tile([128, 4], F32, tag="l2b")
            nc.sync.dma_start(l2b[:], dram["el2b"][li])

            # v-bias broadcast [128, 512] (token-major V bias), once per layer
            vb_ps = b1.tile([128, D], F32, tag="b1")
            nc.tensor.matmul(vb_ps[:], ones_row[:], vbrow[:], start=True, stop=True)
            vb_bc = vbp.tile([128, D], F32, tag="vb_bc")
            nc.scalar.activation(vb_bc[:], vb_ps[:], AF.Copy)

            rc1 = ln_stats_pair(x)
            for s in range(BL):
                sl = slice(s * S, (s + 1) * S)
                h = a4.tile([128, KC, S], F32R, tag="a4")
                ln_apply(rc1, x, s, h)
                # ---- stage 1: q,k (permuted) + v ----
                qkv1 = a8.tile([128, 8, S], F32R, tag="a8")
                v1 = a4.tile([128, KC, S], F32R, tag="a4")
                for m in range(12):
                    ps = b1.tile([128, S], F32, tag="b1")
                    for k in range(KC):
                        nc.tensor.matmul(ps[:], w1[:, k, m * 128:(m + 1) * 128],
                                         h[:, k, :], start=(k == 0), stop=(k == KC - 1))
                    dest = qkv1[:, m, :] if m < 8 else v1[:, m - 8, :]
                    nc.scalar.activation(dest, ps[:], AF.Identity,
                                         bias=w1b[:, m:m + 1])
                # ---- rope: qkv1 (chunks 0-7) -> qkr (natural order) ----
                qkr = a8.tile([128, 8, S], F32R, tag="a8")
                for half in (0, 4):
                    for c in range(2):
                        e = qkv1[:, half + c, :]
                        o = qkv1[:, half + 2 + c, :]
                        r1 = qkr[:, half + c, :]
                        r2 = qkr[:, half + 2 + c, :]
                        t1 = rtp.tile([128, S], F32, tag="rt")
                        nc.vector.tensor_tensor(r1, e, gridc[:, c, :], ALU.mult)
                        nc.vector.tensor_tensor(t1[:], o, grids[:, c, :], ALU.mult)
                        nc.vector.tensor_tensor(r1, r1, t1[:], ALU.subtract)
                        t2 = rtp.tile([128, S], F32, tag="rt")
                        nc.vector.tensor_tensor(r2, e, grids[:, c, :], ALU.mult)
                        nc.vector.tensor_tensor(t2[:], o, gridc[:, c, :], ALU.mult)
                        nc.vector.tensor_tensor(r2, r2, t2[:], ALU.add)
                # ---- stage 2: Q,K ----
                qk2 = qk2p.tile([128, 8, S], F32R, tag="a8")
                for m in range(8):
                    ps = b1.tile([128, S], F32, tag="b1")
                    base = 0 if m < 4 else 4
                    for k in range(KC):
                        nc.tensor.matmul(ps[:], w2[:, k, m * 128:(m + 1) * 128],
                                         qkr[:, base + k, :],
                                         start=(k == 0), stop=(k == KC - 1))
                    nc.scalar.activation(qk2[:, m, :], ps[:], AF.Identity,
                                         bias=w2b[:, m:m + 1])
                # ---- stage 2: V (token-major, with ones column per head) ----
                vloc = vp.tile([128, KC, 8, 65], F32R, tag="vloc")
                for t in range(KC):
                    nc.vector.tensor_copy(vloc[:, t, :, 64], ones8[:])
                for t in range(KC):
                    ps = b1.tile([128, S], F32, tag="b1")
                    for k in range(KC):
                        nc.tensor.matmul(
                            ps[:], v1[:, k, t * 128:(t + 1) * 128],
                            w1[:, k, 2 * D:3 * D],
                            start=(k == 0), stop=(k == KC - 1))
                    nc.vector.tensor_tensor(
                        vloc[:, t, :, 0:64],
                        ps[:].rearrange("p (h d) -> p h d", h=H),
                        vb_bc[:].rearrange("p (h d) -> p h d", h=H), ALU.add)
                # ---- attention heads (paired: exp(h+1) hides under A@V(h)) ----
                o_t = a4.tile([128, KC, S], F32R, tag="a4")
                for h0 in range(0, H, 2):
                    ats = {}
                    for hh in (h0, h0 + 1):
                        rows = slice(64 * (hh % 2), 64 * (hh % 2) + 64)
                        at = atp.tile([128, KC, S], F32R, tag="a4")
                        for c in range(KC):
                            scp = b1.tile([128, S], F32, tag="b1")
                            nc.tensor.matmul(
                                scp[:],
                                qk2[rows, 4 + hh // 2, c * 128:(c + 1) * 128],
                                qk2[rows, hh // 2, :], start=True, stop=True)
                            nc.scalar.activation(at[:, c, :], scp[:], AF.Exp,
                                                 scale=float(1.0 / np.sqrt(DH)))
                        ats[hh] = at
                    for hh in (h0, h0 + 1):
                        rows = slice(64 * (hh % 2), 64 * (hh % 2) + 64)
                        at = ats[hh]
                        ov = b2.tile([65, S], F32, tag="b2")
                        for c in range(KC):
                            nc.tensor.matmul(ov[:], vloc[:, c, hh, :], at[:, c, :],
                                             start=(c == 0), stop=(c == KC - 1))
                        rec = scr.tile([1, S], F32R, tag="scr")
                        nc.vector.reciprocal(rec[:], ov[64:65, :])
                        rb = b2.tile([64, S], F32, tag="b2")
                        nc.tensor.matmul(rb[:], ones_row[:, 0:64], rec[:],
                                         start=True, stop=True)
                        rb_sb = scr.tile([64, S], F32, tag="scr")
                        nc.scalar.activation(rb_sb[:], rb[:], AF.Copy)
                        nc.vector.tensor_tensor(o_t[rows, hh // 2, :],
                                                ov[0:64, :], rb_sb[:], ALU.mult)
                # ---- out-proj + residual ----
                for m in range(KC):
                    ps = b1.tile([128, S], F32, tag="b1")
                    for k in range(KC):
                        nc.tensor.matmul(ps[:], ow[:, k, m * 128:(m + 1) * 128],
                                         o_t[:, k, :], start=(k == 0),
                                         stop=(k == KC - 1))
                    nc.vector.scalar_tensor_tensor(
                        x[:, m, sl], ps[:], owb[:, m:m + 1], x[:, m, sl],
                        ALU.add, ALU.add)
            # ---- phase B: l2 streams in (chunked) once w1 is released ----
            l2 = wpool.tile([128, FC, D], F32R, tag="bigw")
            for kf in range(FC):
                nc.sync.dma_start(l2[:, kf, :], dram["el2T"][li][:, kf, :])
            rc2 = ln_stats_pair(x)
            for s in range(BL):
                sl = slice(s * S, (s + 1) * S)
                h2 = a4.tile([128, KC, S], F32R, tag="a4")
                ln_apply(rc2, x, s, h2)
                # ---- FFN ----
                f2 = [b1.tile([128, S], F32, tag="b1", name=f"f2_{_m}")
                      for _m in range(KC)]
                for kf in range(FC):
                    f1 = b2.tile([128, S], F32, tag="b2")
                    for k in range(KC):
                        nc.tensor.matmul(f1[:], l1[:, k, kf * 128:(kf + 1) * 128],
                                         h2[:, k, :], start=(k == 0),
                                         stop=(k == KC - 1))
                    rt = rtp.tile([128, S], F32R, tag="rt")
                    nc.scalar.activation(rt[:], f1[:], AF.Relu,
                                         bias=l1b[:, kf:kf + 1])
                    for m in range(KC):
                        nc.tensor.matmul(f2[m][:], l2[:, kf, m * 128:(m + 1) * 128],
                                         rt[:], start=(kf == 0), stop=(kf == FC - 1))
                for m in range(KC):
                    nc.vector.scalar_tensor_tensor(
                        x[:, m, sl], f2[m][:], l2b[:, m:m + 1], x[:, m, sl],
                        ALU.add, ALU.add)

        # ---------------- final encoder LN (in-place; affine folded) --------
        me = x
        rcf = ln_stats_pair(x)
        for s in range(BL):
            ln_apply(rcf, x, s,
                     x[:, :, s * S:(s + 1) * S])

        # ---------------- decoder layers ----------------
        for li in range(ld):
            dw = wpool.tile([128, KC, 3 * D], F32R, tag="bigw")
            nc.sync.dma_start(dw[:], dram["dinT"][li])
            dwb = bpool.tile([128, 12], F32, tag="w1b")
            nc.sync.dma_start(dwb[:], dram["dinb"][li])
            dvbrow = bpool.tile([1, D], F32R, tag="vbrow")
            nc.sync.dma_start(dvbrow[:], dram["dvb"][li])
            do = owpool.tile([128, KC, D], F32R, tag="ow")
            nc.sync.dma_start(do[:], dram["dowT"][li])
            dob = bpool.tile([128, 4], F32, tag="owb")
            nc.sync.dma_start(dob[:], dram["dowb"][li])
            m1 = wpool.tile([128, KC, M], F32R, tag="bigw")
            nc.sync.dma_start(m1[:], dram["dm1T"][li])
            m1b = bpool.tile([128, 16], F32, tag="l1b")
            nc.sync.dma_start(m1b[:], dram["dm1b"][li])
            m2b = bpool.tile([128, 4], F32, tag="l2b")
            nc.sync.dma_start(m2b[:], dram["dm2b"][li])

            vb_ps = b1.tile([128, D], F32, tag="b1")
            nc.tensor.matmul(vb_ps[:], ones_row[:], dvbrow[:], start=True, stop=True)
            vb_bc = vbp.tile([128, D], F32, tag="vb_bc")
            nc.scalar.activation(vb_bc[:], vb_ps[:], AF.Copy)

            # LN(p) -> q_ln ; Q projection (all samples at once, N=BL)
            q_ln = smalls.tile([128, KC, BL], F32R, tag="q_ln")
            ln(p, slice(None), BL, q_ln, slice(None))
            qps = b1.tile([128, KC, BL], F32, tag="b1")
            for m in range(KC):
                for k in range(KC):
                    nc.tensor.matmul(qps[:, m, :],
                                     c32(dw[:, k, m * 128:(m + 1) * 128]),
                                     c32(q_ln[:, k, :]), start=(k == 0),
                                     stop=(k == KC - 1))
            q_sb = smalls.tile([128, KC, BL], F32R, tag="q_sb")
            for m in range(KC):
                nc.scalar.activation(q_sb[:, m, :], qps[:, m, :], AF.Identity,
                                     bias=dwb[:, m:m + 1])
            o_d = smalls.tile([128, KC, BL], F32R, tag="o_d")
            for s in range(BL):
                sl = slice(s * S, (s + 1) * S)
                # K (feature-major) and V' (token-major) over morph_enc
                k_sb = a4.tile([128, KC, S], F32R, tag="a4")
                for m in range(KC):
                    ps = b1.tile([128, S], F32, tag="b1")
                    for k in range(KC):
                        nc.tensor.matmul(
                            ps[:], dw[:, k, D + m * 128:D + (m + 1) * 128],
                            me[:, k, sl], start=(k == 0), stop=(k == KC - 1))
                    nc.scalar.activation(k_sb[:, m, :], ps[:], AF.Identity,
                                         bias=dwb[:, 4 + m:5 + m])
                vloc = vp.tile([128, KC, 8, 65], F32R, tag="vloc")
                for t in range(KC):
                    nc.vector.tensor_copy(vloc[:, t, :, 64], ones8[:])
                for t in range(KC):
                    ps = b1.tile([128, S], F32, tag="b1")
                    for k in range(KC):
                        nc.tensor.matmul(
                            ps[:], me[:, k, s * S + t * 128:s * S + (t + 1) * 128],
                            dw[:, k, 2 * D:3 * D],
                            start=(k == 0), stop=(k == KC - 1))
                    nc.vector.tensor_tensor(
                        vloc[:, t, :, 0:64],
                        ps[:].rearrange("p (h d) -> p h d", h=H),
                        vb_bc[:].rearrange("p (h d) -> p h d", h=H), ALU.add)
                scp = b1.tile([128, KC, H], F32, tag="b1")
                for hh in range(H):
                    rows = slice(64 * (hh % 2), 64 * (hh % 2) + 64)
                    for c in range(KC):
                        nc.tensor.matmul(
                            scp[:, c, hh:hh + 1],
                            c32(k_sb[rows, hh // 2, c * 128:(c + 1) * 128]),
                            c32(q_sb[rows, hh // 2, s:s + 1]),
                            start=True, stop=True)
                at = smalls.tile([128, KC, H], F32R, tag="at_d")
                nc.scalar.activation(at[:], scp[:], AF.Exp,
                                     scale=float(1.0 / np.sqrt(DH)))
                ov = b2.tile([65, H], F32, tag="b2")
                for hh in range(H):
                    for c in range(KC):
                        nc.tensor.matmul(ov[:, hh:hh + 1], c32(vloc[:, c, hh, :]),
                                         c32(at[:, c, hh:hh + 1]),
                                         start=(c == 0), stop=(c == KC - 1))
                rec = scr.tile([1, H], F32R, tag="scr")
                nc.vector.reciprocal(rec[:], ov[64:65, :])
                rb = b2.tile([64, H], F32, tag="b2")
                nc.tensor.matmul(rb[:], c32(ones_row[:, 0:64]), c32(rec[:]),
                                 start=True, stop=True)
                rb_sb = scr.tile([64, H], F32, tag="scr")
                nc.scalar.activation(rb_sb[:], rb[:], AF.Copy)
                for hh in range(H):
                    rows = slice(64 * (hh % 2), 64 * (hh % 2) + 64)
                    nc.vector.tensor_tensor(o_d[rows, hh // 2, s:s + 1],
                                            ov[0:64, hh:hh + 1],
                                            rb_sb[:, hh:hh + 1], ALU.mult)
            # out-proj + residual into p
            ops = b1.tile([128, KC, BL], F32, tag="b1")
            for m in range(KC):
                for k in range(KC):
                    nc.tensor.matmul(ops[:, m, :],
                                     c32(do[:, k, m * 128:(m + 1) * 128]),
                                     c32(o_d[:, k, :]), start=(k == 0),
                                     stop=(k == KC - 1))
            for m in range(KC):
                nc.vector.scalar_tensor_tensor(
                    p[:, m, :], ops[:, m, :], dob[:, m:m + 1], p[:, m, :],
                    ALU.add, ALU.add)
            # FFN on p (m2 streams in chunked once dw releases its slot)
            m2 = wpool.tile([128, MC, D], F32R, tag="bigw")
            for kf in range(MC):
                nc.sync.dma_start(m2[:, kf, :], dram["dm2T"][li][:, kf, :])
            h2d = smalls.tile([128, KC, BL], F32R, tag="q_ln")
            ln(p, slice(None), BL, h2d, slice(None))
            mh = smalls.tile([128, MC, BL], F32R, tag="mh")
            for mm_ in range(MC):
                ps = b1.tile([128, BL], F32, tag="b1")
                for k in range(KC):
                    nc.tensor.matmul(ps[:], c32(m1[:, k, mm_ * 128:(mm_ + 1) * 128]),
                                     c32(h2d[:, k, :]), start=(k == 0),
                                     stop=(k == KC - 1))
                nc.scalar.activation(mh[:, mm_, :], ps[:], AF.Relu,
                                     bias=m1b[:, mm_:mm_ + 1])
            m2ps = b1.tile([128, KC, BL], F32, tag="b1")
            for m in range(KC):
                for kf in range(MC):
                    nc.tensor.matmul(m2ps[:, m, :],
                                     c32(m2[:, kf, m * 128:(m + 1) * 128]),
                                     c32(mh[:, kf, :]), start=(kf == 0),
                                     stop=(kf == MC - 1))
            for m in range(KC):
                nc.vector.scalar_tensor_tensor(
                    p[:, m, :], m2ps[:, m, :], m2b[:, m:m + 1], p[:, m, :],
                    ALU.add, ALU.add)

        # ---------------- head ----------------
        hw = smalls.tile([128, KC], F32R, tag="hw")
        hb = smalls.tile([1, 1], F32, tag="hb")
        nc.sync.dma_start(hw[:], dram["hwT"][:])
        nc.sync.dma_start(hb[:], dram["hb"][:])
        hg = smalls.tile([128, KC, BL], F32R, tag="q_ln")
        ln(p, slice(None), BL, hg, slice(None))
        hps = b2.tile([1, BL], F32, tag="b2")
        for k in range(KC):
            nc.tensor.matmul(hps[:], c32(hw[:, k:k + 1]), c32(hg[:, k, :]),
                             start=(k == 0), stop=(k == KC - 1))
        y_sb = smalls.tile([1, BL], F32, tag="y_sb")
        nc.scalar.activation(y_sb[:], hps[:], AF.Sigmoid, bias=hb[:])
        nc.sync.dma_start(y_dram[:], y_sb[:])


# ----------------------------------------------------------------------------
# entry point
# ----------------------------------------------------------------------------

_NC_CACHE = {}


def kernel(**inputs):
    return _run(inputs, LE, LD)


def _run(inputs, le, ld, trace=False):
    w = prep_weights(inputs, le, ld)
    morph = np.asarray(inputs["morph"], np.float32)
    pose = np.asarray(inputs["pose"], np.float32)
    in_maps = []
    for c in range(NCORES):
        im = dict(w)
        mo = morph[c * BL:(c + 1) * BL]                 # [BL, S, 3]
        im["morphT"] = np.ascontiguousarray(
            mo.transpose(2, 0, 1).reshape(3, T))
        im["poseT"] = np.ascontiguousarray(pose[c * BL:(c + 1) * BL].T)
        in_maps.append(im)

    if ("nc", le, ld) not in _NC_CACHE:
        _NC_CACHE[("nc", le, ld)] = build(le, ld)
    nc = _NC_CACHE[("nc", le, ld)]
    res = run_bass_kernel_spmd(nc, in_maps, core_ids=list(range(NCORES)),
                               trace=trace)
    out = np.zeros((B, 1), np.float32)
    for c in range(NCORES):
        out[c * BL:(c + 1) * BL, 0] = res.results[c]["y"][0]
    if trace:
        return out, res
    return out



# revision 9
# speedup vs baseline: 1.3725x; 1.3725x over previous
"""Trainium2 Bass kernel for nn_ReachabilityClassifierTransformer.

Data-parallel over batch: 16 samples / 8 cores = 2 samples per core.
Each core runs the full network (6-layer encoder + 4-layer decoder + head)
on its 2 samples. No collectives.

v2 (fp16 pipeline):
  - All matmul operands fp16 (stationary weights get Fast-Weight-Load; DVE
    elementwise ops run in 2x mode; DMA traffic halved). PSUM stays fp32.
  - V path folded on host: V = h @ (Wv2 Wv1)^T + bv  (no rope between the
    two V projections, so the double-projection quirk collapses).
  - Stage-1 q/k biases folded through rope into per-position bias tensors
    C_q/C_k = W2 @ rope(b1) + b2, added at the stage-2 PSUM copy (rope is
    linear, rotation depends only on position).
  - No Sqrt / no DVE reciprocal anywhere: LN rsqrt = exp(-0.5 ln(v+eps)),
    softmax 1/denom = exp(-ln denom) broadcast via PE.  ln/exp/copy/relu/
    square all live in one activation table set -> no table switches.
  - Per-layer phase order interleaves the two samples so rope (DVE) and
    softmax exp (Act) hide under the other sample's matmuls.

Device layout conventions (per core):
  - Activations FEATURE-MAJOR in SBUF: tile [128, KC, T] holds X.T.
  - Weights pre-transposed on host to [in_feat, out_feat], laid out
    [128, KC_in, O] (partition = in-feature % 128).
  - matmul(out_psum[M,N], lhsT=[K,M], rhs=[K,N]) computes lhsT.T @ rhs.
  - Encoder stage-1 q,k output features are de-interleaved (even feats then
    odd feats) via host-side column permutation, so RoPE becomes contiguous
    block ops; the roped result is in natural (concatenated) order.
  - Softmax: scores computed transposed (S.T = K_h @ Q_h.T per chunk),
    exp'd without max subtraction (|scores/8| < 1 for this model), and the
    denominator comes free from a ones-column appended to V.
"""
import numpy as np

import concourse.bass as bass
import concourse.mybir as mybir
import concourse.tile as tile
from concourse import bacc
from concourse.bass_utils import run_bass_kernel_spmd

AF = mybir.ActivationFunctionType
ALU = mybir.AluOpType
F32 = mybir.dt.float32
F16 = mybir.dt.float16

B, S, D, FF, H, LE, LD, M = 16, 512, 512, 2048, 8, 6, 4, 2048
ROPE_BASE = 10000.0
LN_EPS = 1e-5
NCORES = 8
BL = B // NCORES          # 2 samples per core
T = BL * S                # 1024 tokens per core
KC = D // 128             # 4 feature chunks
FC = FF // 128            # 16
MC = M // 128             # 16
DH = D // H               # 64


# ----------------------------------------------------------------------------
# host-side helpers
# ----------------------------------------------------------------------------

def _chunked(wT):
    """[Din, O] -> [128, Din//128, O] contiguous fp16."""
    Din, O = wT.shape
    return np.ascontiguousarray(
        wT.reshape(Din // 128, 128, O).transpose(1, 0, 2)).astype(np.float16)


def _bias_cols(b):
    """[O] -> [128, O//128]  (column per 128-chunk), fp32."""
    O = b.shape[0]
    return np.ascontiguousarray(b.reshape(O // 128, 128).T).astype(np.float32)


_DEINT = np.concatenate([np.arange(0, D, 2), np.arange(1, D, 2)])  # de-interleave


def prep_weights(inp, le=LE, ld=LD):
    """Host-side weight prep -> dict of arrays shared by all cores."""
    out = {}
    g = {k: np.asarray(v, np.float64) for k, v in inp.items()}

    out["mpwT"] = np.ascontiguousarray(g["morph_proj_w"].T).astype(np.float16)
    out["mpb"] = _bias_cols(g["morph_proj_b"])                     # [128, 4]
    out["ppwT"] = np.ascontiguousarray(g["pose_proj_w"].T).astype(np.float16)
    out["ppb"] = _bias_cols(g["pose_proj_b"])

    # rope grids, de-interleaved frequency order: [128, 2, 512] fp16
    freq = 1.0 / ROPE_BASE ** (np.arange(0, D, 2, dtype=np.float64) / D)
    ang = np.outer(np.arange(S, dtype=np.float64), freq)           # [512, 256]
    cosT = np.cos(ang).T                                           # [256, S]
    sinT = np.sin(ang).T
    out["gridc"] = _chunked(cosT.reshape(256, S))
    out["grids"] = _chunked(sinT.reshape(256, S))

    e_w1, e_w2, e_wv, e_cqk, e_vb = [], [], [], [], []
    e_ow, e_owb, e_l1, e_l1b, e_l2, e_l2b = [], [], [], [], [], []
    for i in range(le):
        w1 = g["enc_in_w"][i] * g["enc_n1_g"][i][None, :]          # fold n1 g
        b1 = g["enc_in_b"][i] + g["enc_in_w"][i] @ g["enc_n1_b"][i]
        # stage-1 q,k only, de-interleaved output columns
        perm = np.concatenate([_DEINT, D + _DEINT])
        e_w1.append(_chunked(np.ascontiguousarray(w1[perm].T)))    # [128,4,1024]
        # stage-2 q,k (natural order, raw weights - the faithful quirk)
        w2 = g["enc_in_w"][i][: 2 * D]                             # Wq;Wk
        e_w2.append(_chunked(np.ascontiguousarray(w2.T)))          # [128,4,1024]
        # stage-2 bias tensors: C = W2 @ rope(b1) + b2   [512, S] each
        bq = b1[:D][_DEINT]                                        # [even; odd]
        bk = b1[D:2 * D][_DEINT]
        rb_q = np.concatenate([bq[:256, None] * cosT - bq[256:, None] * sinT,
                               bq[:256, None] * sinT + bq[256:, None] * cosT])
        rb_k = np.concatenate([bk[:256, None] * cosT - bk[256:, None] * sinT,
                               bk[:256, None] * sinT + bk[256:, None] * cosT])
        Cq = g["enc_in_w"][i][:D] @ rb_q + g["enc_in_b"][i][:D][:, None]
        Ck = g["enc_in_w"][i][D:2 * D] @ rb_k \
            + g["enc_in_b"][i][D:2 * D][:, None]
        C = np.concatenate([Cq, Ck], axis=0)                       # [1024, S]
        e_cqk.append(_chunked(C))                                  # [128,8,S]
        # V folded: V = h @ (Wv2 Wv1_f).T + (Wv2 bv1_f + bv2)
        Wv1f = w1[2 * D:]
        bv1f = b1[2 * D:]
        Wv2 = g["enc_in_w"][i][2 * D:]
        bv2 = g["enc_in_b"][i][2 * D:]
        e_wv.append(_chunked(np.ascontiguousarray((Wv2 @ Wv1f).T)))
        e_vb.append((Wv2 @ bv1f + bv2)[None, :].astype(np.float16))  # [1,512]
        e_ow.append(_chunked(np.ascontiguousarray(g["enc_out_w"][i].T)))
        e_owb.append(_bias_cols(g["enc_out_b"][i]))
        l1 = g["enc_l1_w"][i] * g["enc_n2_g"][i][None, :]
        l1b = g["enc_l1_b"][i] + g["enc_l1_w"][i] @ g["enc_n2_b"][i]
        e_l1.append(_chunked(np.ascontiguousarray(l1.T)))          # [128,4,2048]
        e_l1b.append(_bias_cols(l1b))                              # [128,16]
        e_l2.append(_chunked(np.ascontiguousarray(g["enc_l2_w"][i].T)))
        e_l2b.append(_bias_cols(g["enc_l2_b"][i]))                 # [128,4]
    out["ew1T"] = np.stack(e_w1) if le else np.zeros((0, 128, KC, 2 * D), np.float16)
    out["ew2T"] = np.stack(e_w2) if le else np.zeros((0, 128, KC, 2 * D), np.float16)
    out["ewvT"] = np.stack(e_wv) if le else np.zeros((0, 128, KC, D), np.float16)
    out["ecqk"] = np.stack(e_cqk) if le else np.zeros((0, 128, 8, S), np.float16)
    out["evb"] = np.stack(e_vb) if le else np.zeros((0, 1, D), np.float16)
    out["eowT"] = np.stack(e_ow) if le else np.zeros((0, 128, KC, D), np.float16)
    out["eowb"] = np.stack(e_owb) if le else np.zeros((0, 128, 4), np.float32)
    out["el1T"] = np.stack(e_l1) if le else np.zeros((0, 128, KC, FF), np.float16)
    out["el1b"] = np.stack(e_l1b) if le else np.zeros((0, 128, 16), np.float32)
    out["el2T"] = np.stack(e_l2) if le else np.zeros((0, 128, FC, D), np.float16)
    out["el2b"] = np.stack(e_l2b) if le else np.zeros((0, 128, 4), np.float32)

    d_in, d_inb, d_vb, d_ow, d_owb = [], [], [], [], []
    d_m1, d_m1b, d_m2, d_m2b = [], [], [], []
    for i in range(ld):
        w = g["dec_in_w"][i].copy()
        b = g["dec_in_b"][i].copy()
        w[:D] = w[:D] * g["dec_n1_g"][i][None, :]                  # Wq <- dec_n1
        b[:D] = b[:D] + g["dec_in_w"][i][:D] @ g["dec_n1_b"][i]
        w[D:] = w[D:] * g["enc_final_g"][None, :]                  # Wk,Wv <- enc_final
        b[D:] = b[D:] + g["dec_in_w"][i][D:] @ g["enc_final_b"]
        d_in.append(_chunked(np.ascontiguousarray(w.T)))           # [128,4,1536]
        d_inb.append(_bias_cols(b))
        d_vb.append(b[2 * D:][None, :].astype(np.float16))         # [1,512]
        d_ow.append(_chunked(np.ascontiguousarray(g["dec_out_w"][i].T)))
        d_owb.append(_bias_cols(g["dec_out_b"][i]))
        m1 = g["dec_m1_w"][i] * g["dec_n2_g"][i][None, :]
        m1b = g["dec_m1_b"][i] + g["dec_m1_w"][i] @ g["dec_n2_b"][i]
        d_m1.append(_chunked(np.ascontiguousarray(m1.T)))          # [128,4,2048]
        d_m1b.append(_bias_cols(m1b))
        d_m2.append(_chunked(np.ascontiguousarray(g["dec_m2_w"][i].T)))
        d_m2b.append(_bias_cols(g["dec_m2_b"][i]))
    out["dinT"] = np.stack(d_in) if ld else np.zeros((0, 128, KC, 3 * D), np.float16)
    out["dinb"] = np.stack(d_inb) if ld else np.zeros((0, 128, 12), np.float32)
    out["dvb"] = np.stack(d_vb) if ld else np.zeros((0, 1, D), np.float16)
    out["dowT"] = np.stack(d_ow) if ld else np.zeros((0, 128, KC, D), np.float16)
    out["dowb"] = np.stack(d_owb) if ld else np.zeros((0, 128, 4), np.float32)
    out["dm1T"] = np.stack(d_m1) if ld else np.zeros((0, 128, KC, M), np.float16)
    out["dm1b"] = np.stack(d_m1b) if ld else np.zeros((0, 128, 16), np.float32)
    out["dm2T"] = np.stack(d_m2) if ld else np.zeros((0, 128, MC, D), np.float16)
    out["dm2b"] = np.stack(d_m2b) if ld else np.zeros((0, 128, 4), np.float32)

    hw = (g["head_w"] * g["head_g"][None, :])[0]                   # [512]
    out["hwT"] = _bias_cols(hw).astype(np.float16)                 # [128, 4]
    out["hb"] = (g["head_bias"] + g["head_w"] @ g["head_b"]).reshape(1, 1).astype(np.float32)
    return out


# ----------------------------------------------------------------------------
# device program
# ----------------------------------------------------------------------------

def build(le=LE, ld=LD):
    nc = bacc.Bacc(None, target_bir_lowering=False)

    dram = {}

    def din(name, shape, dt=F16):
        dram[name] = nc.dram_tensor(name, list(shape), dt, kind="ExternalInput")
        return dram[name]

    # shared weights
    din("mpwT", [3, 512]); din("mpb", [128, 4], F32)
    din("ppwT", [9, 512]); din("ppb", [128, 4], F32)
    din("gridc", [128, 2, S]); din("grids", [128, 2, S])
    din("ew1T", [le, 128, KC, 2 * D])
    din("ew2T", [le, 128, KC, 2 * D])
    din("ewvT", [le, 128, KC, D])
    din("ecqk", [le, 128, 8, S])
    din("evb", [le, 1, D])
    din("eowT", [le, 128, KC, D]); din("eowb", [le, 128, 4], F32)
    din("el1T", [le, 128, KC, FF]); din("el1b", [le, 128, 16], F32)
    din("el2T", [le, 128, FC, D]); din("el2b", [le, 128, 4], F32)
    din("dinT", [ld, 128, KC, 3 * D]); din("dinb", [ld, 128, 12], F32)
    din("dvb", [ld, 1, D])
    din("dowT", [ld, 128, KC, D]); din("dowb", [ld, 128, 4], F32)
    din("dm1T", [ld, 128, KC, M]); din("dm1b", [ld, 128, 16], F32)
    din("dm2T", [ld, 128, MC, D]); din("dm2b", [ld, 128, 4], F32)
    din("hwT", [128, KC]); din("hb", [1, 1], F32)
    # per-core inputs
    din("morphT", [3, T])
    din("poseT", [9, BL])
    y = nc.dram_tensor("y", [1, BL], F32, kind="ExternalOutput")

    with tile.TileContext(nc) as tc:
        _build_body(nc, tc, dram, y, le, ld)
    nc.compile()
    return nc


def _build_body(nc, tc, dram, y_dram, le, ld):
    import contextlib
    ctx = contextlib.ExitStack()
    with ctx:
        ctx.enter_context(nc.allow_low_precision(
            reason="fp16 matmul operands / activations are intentional"))
        persist = ctx.enter_context(tc.tile_pool(name="persist", bufs=1))
        wpool = ctx.enter_context(tc.tile_pool(name="wpool", bufs=2))
        w2pool = ctx.enter_context(tc.tile_pool(name="w2pool", bufs=1))
        owpool = ctx.enter_context(tc.tile_pool(name="owpool", bufs=1))
        vwpool = ctx.enter_context(tc.tile_pool(name="vwpool", bufs=1))
        cqpool = ctx.enter_context(tc.tile_pool(name="cqpool", bufs=1))
        bpool = ctx.enter_context(tc.tile_pool(name="bpool", bufs=2))
        a4 = ctx.enter_context(tc.tile_pool(name="a4", bufs=4))
        hp = ctx.enter_context(tc.tile_pool(name="hp", bufs=2))
        a8 = ctx.enter_context(tc.tile_pool(name="a8", bufs=3))
        qk2p = ctx.enter_context(tc.tile_pool(name="qk2p", bufs=2))
        vp = ctx.enter_context(tc.tile_pool(name="vp", bufs=2))
        rtp = ctx.enter_context(tc.tile_pool(name="rtp", bufs=3))
        vbp = ctx.enter_context(tc.tile_pool(name="vbp", bufs=1))
        scr = ctx.enter_context(tc.tile_pool(name="scr", bufs=3))
        invp = ctx.enter_context(tc.tile_pool(name="invp", bufs=2))
        rcp = ctx.enter_context(tc.tile_pool(name="rcp", bufs=2))
        smalls = ctx.enter_context(tc.tile_pool(name="smalls", bufs=2))
        b1 = ctx.enter_context(tc.tile_pool(name="b1", bufs=4, space="PSUM"))
        b2 = ctx.enter_context(tc.tile_pool(name="b2", bufs=3, space="PSUM"))

        # ---------------- persistent tiles ----------------
        x = persist.tile([128, KC, T], F16)           # residual stream (X.T)
        gridc = persist.tile([128, 2, S], F16)
        grids = persist.tile([128, 2, S], F16)
        ones128 = persist.tile([128, 1], F16)
        ones_row = persist.tile([1, 128], F16)
        ones8 = persist.tile([128, 8], F16)
        eps_t = persist.tile([1, 1], F32)
        p = persist.tile([128, KC, BL], F16)          # decoder latent p.T
        nc.sync.dma_start(gridc[:], dram["gridc"][:])
        nc.sync.dma_start(grids[:], dram["grids"][:])
        stage_f16 = rtp.tile([128, 128], F16, tag="rt")
        nc.vector.memset(stage_f16[:], 1.0)
        nc.vector.tensor_copy(ones128[:], stage_f16[:, 0:1])
        nc.vector.tensor_copy(ones_row[:], stage_f16[0:1, :])
        nc.vector.tensor_copy(ones8[:], stage_f16[:, 0:8])
        nc.vector.memset(eps_t[:], LN_EPS)

        def ln_small(x_tile, n_tok, h_out):
            """h_out = LayerNorm_features(x_tile) for tiny n_tok (decoder)."""
            sq = smalls.tile([128, KC, n_tok], F16, tag="sq_d")
            for k in range(KC):
                nc.vector.tensor_tensor(sq[:, k, :], x_tile[:, k, :],
                                        x_tile[:, k, :], ALU.mult)
            sum_ps = b2.tile([1, n_tok], F32, tag="b2")
            sq_ps = b2.tile([1, n_tok], F32, tag="b2")
            for k in range(KC):
                nc.tensor.matmul(sum_ps[:], ones128[:], x_tile[:, k, :],
                                 start=(k == 0), stop=(k == KC - 1))
            for k in range(KC):
                nc.tensor.matmul(sq_ps[:], ones128[:], sq[:, k, :],
                                 start=(k == 0), stop=(k == KC - 1))
            ms = scr.tile([1, n_tok], F32, tag="scr")
            t2 = scr.tile([1, n_tok], F32, tag="scr")
            rc = scr.tile([1, 2, n_tok], F16, tag="scr_rc")
            nc.scalar.activation(ms[:], sum_ps[:], AF.Copy, scale=1.0 / D)
            nc.vector.tensor_tensor(t2[:], ms[:], ms[:], ALU.mult)      # m^2
            nc.vector.scalar_tensor_tensor(
                t2[:], sq_ps[:], 1.0 / D, t2[:], ALU.mult, ALU.subtract)
            # r = exp(-0.5 ln(var + eps))
            nc.scalar.activation(t2[:], t2[:], AF.Ln, bias=eps_t[:])
            nc.scalar.activation(rc[:, 0, :], t2[:], AF.Exp, scale=-0.5)
            nc.vector.scalar_tensor_tensor(
                rc[:, 1, :], ms[:], -1.0, rc[:, 0, :], ALU.mult, ALU.mult)
            r_ps = b1.tile([128, n_tok], F32, tag="b1")
            c_ps = b1.tile([128, n_tok], F32, tag="b1")
            nc.tensor.matmul(r_ps[:], ones_row[:], rc[:, 0, :],
                             start=True, stop=True)
            nc.tensor.matmul(c_ps[:], ones_row[:], rc[:, 1, :],
                             start=True, stop=True)
            for k in range(KC):
                nc.vector.tensor_tensor(h_out[:, k, :], x_tile[:, k, :],
                                        r_ps[:], ALU.mult)
                nc.vector.tensor_tensor(h_out[:, k, :], h_out[:, k, :],
                                        c_ps[:], ALU.add)

        def ln_stats(x_tile, s):
            """Per-sample LN stats -> rc [1, 2, S] fp16 (r, c)."""
            sl = slice(s * S, (s + 1) * S)
            sq = a4.tile([128, KC, S], F16, tag="sq", bufs=2)
            nc.vector.tensor_tensor(sq[:], x_tile[:, :, sl], x_tile[:, :, sl],
                                    ALU.mult)
            sum_ps = b2.tile([1, S], F32, tag="b2")
            sq_ps = b2.tile([1, S], F32, tag="b2")
            for k in range(KC):
                nc.tensor.matmul(sum_ps[:], ones128[:], x_tile[:, k, sl],
                                 start=(k == 0), stop=(k == KC - 1))
            for k in range(KC):
                nc.tensor.matmul(sq_ps[:], ones128[:], sq[:, k, :],
                                 start=(k == 0), stop=(k == KC - 1))
            ms = scr.tile([1, S], F32, tag="scr")
            t2 = scr.tile([1, S], F32, tag="scr")
            rc = rcp.tile([1, 2, S], F16, tag="rc")
            nc.scalar.activation(ms[:], sum_ps[:], AF.Copy, scale=1.0 / D)
            nc.vector.tensor_tensor(t2[:], ms[:], ms[:], ALU.mult)
            nc.vector.scalar_tensor_tensor(
                t2[:], sq_ps[:], 1.0 / D, t2[:], ALU.mult, ALU.subtract)
            nc.scalar.activation(t2[:], t2[:], AF.Ln, bias=eps_t[:])
            nc.scalar.activation(rc[:, 0, :], t2[:], AF.Exp, scale=-0.5)
            nc.vector.scalar_tensor_tensor(
                rc[:, 1, :], ms[:], -1.0, rc[:, 0, :], ALU.mult, ALU.mult)
            return rc

        def ln_apply(rc, x_tile, s, h_out):
            """h_out[:, :, :] = x[:, :, s] * r + c  (r,c broadcast via PE)."""
            sl = slice(s * S, (s + 1) * S)
            r_ps = b1.tile([128, S], F32, tag="b1")
            c_ps = b1.tile([128, S], F32, tag="b1")
            nc.tensor.matmul(r_ps[:], ones_row[:], rc[:, 0, :],
                             start=True, stop=True)
            nc.tensor.matmul(c_ps[:], ones_row[:], rc[:, 1, :],
                             start=True, stop=True)
            rcb = rcp.tile([128, 2, S], F16, tag="rcb")
            nc.scalar.activation(rcb[:, 0, :], r_ps[:], AF.Copy)
            nc.scalar.activation(rcb[:, 1, :], c_ps[:], AF.Copy)
            for k in range(KC):
                nc.vector.tensor_tensor(h_out[:, k, :], x_tile[:, k, sl],
                                        rcb[:, 0, :], ALU.mult)
                nc.vector.tensor_tensor(h_out[:, k, :], h_out[:, k, :],
                                        rcb[:, 1, :], ALU.add)

        # ---------------- morph projection -> x ----------------
        morpht = rtp.tile([3, T], F16, tag="morph", bufs=1)
        nc.sync.dma_start(morpht[:], dram["morphT"][:])
        mpw = rtp.tile([3, 512], F16, tag="rt")
        mpb = rtp.tile([128, 4], F32, tag="rtb")
        nc.sync.dma_start(mpw[:], dram["mpwT"][:])
        nc.sync.dma_start(mpb[:], dram["mpb"][:])
        for m in range(KC):
            for s in range(BL):
                ps = b1.tile([128, S], F32, tag="b1")
                nc.tensor.matmul(ps[:], mpw[:, m * 128:(m + 1) * 128],
                                 morpht[:, s * S:(s + 1) * S], start=True, stop=True)
                nc.scalar.activation(x[:, m, s * S:(s + 1) * S], ps[:], AF.Relu,
                                     bias=mpb[:, m:m + 1])

        # ---------------- pose projection -> p ----------------
        poset = rtp.tile([9, BL], F16, tag="rt")
        ppw = rtp.tile([9, 512], F16, tag="rt")
        ppb = rtp.tile([128, 4], F32, tag="rtb")
        nc.sync.dma_start(poset[:], dram["poseT"][:])
        nc.sync.dma_start(ppw[:], dram["ppwT"][:])
        nc.sync.dma_start(ppb[:], dram["ppb"][:])
        pps = b1.tile([128, KC, BL], F32, tag="b1")
        for m in range(KC):
            nc.tensor.matmul(pps[:, m, :], ppw[:, m * 128:(m + 1) * 128],
                             poset[:], start=True, stop=True)
        for m in range(KC):
            nc.scalar.activation(p[:, m, :], pps[:, m, :], AF.Relu,
                                 bias=ppb[:, m:m + 1])

        # ---------------- encoder layers ----------------
        for li in range(le):
            w1 = wpool.tile([128, KC, 2 * D], F16, tag="bigw")
            nc.sync.dma_start(w1[:], dram["ew1T"][li])
            w2 = w2pool.tile([128, KC, 2 * D], F16, tag="w2")
            nc.sync.dma_start(w2[:], dram["ew2T"][li])
            wv = vwpool.tile([128, KC, D], F16, tag="wv")
            nc.sync.dma_start(wv[:], dram["ewvT"][li])
            cq = cqpool.tile([128, 8, S], F16, tag="cq")
            nc.sync.dma_start(cq[:], dram["ecqk"][li])
            vbrow = bpool.tile([1, D], F16, tag="vbrow")
            nc.sync.dma_start(vbrow[:], dram["evb"][li])
            ow = owpool.tile([128, KC, D], F16, tag="ow")
            nc.sync.dma_start(ow[:], dram["eowT"][li])
            owb = bpool.tile([128, 4], F32, tag="owb")
            nc.sync.dma_start(owb[:], dram["eowb"][li])
            l1 = wpool.tile([128, KC, FF], F16, tag="bigw")
            nc.sync.dma_start(l1[:], dram["el1T"][li])
            l1b = bpool.tile([128, 16], F32, tag="l1b")
            nc.sync.dma_start(l1b[:], dram["el1b"][li])
            l2b = bpool.tile([128, 4], F32, tag="l2b")
            nc.sync.dma_start(l2b[:], dram["el2b"][li])

            # v-bias broadcast [128, 512], once per layer
            vb_ps = b1.tile([128, D], F32, tag="b1")
            nc.tensor.matmul(vb_ps[:], ones_row[:], vbrow[:], start=True, stop=True)
            vb_bc = vbp.tile([128, D], F16, tag="vb_bc")
            nc.scalar.activation(vb_bc[:], vb_ps[:], AF.Copy)

            qkrs, vlocs = {}, {}
            # ---- phase A1 per sample: LN1, stage1 q,k; V; rope ----
            for s in range(BL):
                rc1 = ln_stats(x, s)
                h = hp.tile([128, KC, S], F16, tag="h")
                ln_apply(rc1, x, s, h)
                qkv1 = a8.tile([128, 8, S], F16, tag="a8")
                for m in range(8):
                    ps = b1.tile([128, S], F32, tag="b1")
                    for k in range(KC):
                        nc.tensor.matmul(ps[:], w1[:, k, m * 128:(m + 1) * 128],
                                         h[:, k, :], start=(k == 0), stop=(k == KC - 1))
                    nc.scalar.activation(qkv1[:, m, :], ps[:], AF.Copy)
                # V token-major (+ones col), from h directly (folded weights)
                vloc = vp.tile([128, KC, 8, 65], F16, tag="vloc")
                for t in range(KC):
                    nc.vector.tensor_copy(vloc[:, t, :, 64], ones8[:])
                for t in range(KC):
                    ps = b1.tile([128, S], F32, tag="b1")
                    for k in range(KC):
                        nc.tensor.matmul(
                            ps[:], h[:, k, t * 128:(t + 1) * 128],
                            wv[:, k, :], start=(k == 0), stop=(k == KC - 1))
                    nc.vector.tensor_tensor(
                        vloc[:, t, :, 0:64],
                        ps[:].rearrange("p (h d) -> p h d", h=H),
                        vb_bc[:].rearrange("p (h d) -> p h d", h=H), ALU.add)
                vlocs[s] = vloc
                # rope: qkv1 (de-interleaved) -> qkr (natural order), DVE fp16
                qkr = a8.tile([128, 8, S], F16, tag="a8")
                for half in (0, 4):
                    for c in range(2):
                        e = qkv1[:, half + c, :]
                        o = qkv1[:, half + 2 + c, :]
                        r1 = qkr[:, half + c, :]
                        r2 = qkr[:, half + 2 + c, :]
                        t1 = rtp.tile([128, S], F16, tag="rt")
                        nc.vector.tensor_tensor(r1, e, gridc[:, c, :], ALU.mult)
                        nc.vector.tensor_tensor(t1[:], o, grids[:, c, :], ALU.mult)
                        nc.vector.tensor_tensor(r1, r1, t1[:], ALU.subtract)
                        t2 = rtp.tile([128, S], F16, tag="rt")
                        nc.vector.tensor_tensor(r2, e, grids[:, c, :], ALU.mult)
                        nc.vector.tensor_tensor(t2[:], o, gridc[:, c, :], ALU.mult)
                        nc.vector.tensor_tensor(r2, r2, t2[:], ALU.add)
                qkrs[s] = qkr

            # ---- phase A2 per sample: stage2, attention, out-proj ----
            for s in range(BL):
                sl = slice(s * S, (s + 1) * S)
                qkr, vloc = qkrs[s], vlocs[s]
                qk2 = qk2p.tile([128, 8, S], F16, tag="qk2")
                for m in range(8):
                    ps = b1.tile([128, S], F32, tag="b1")
                    base = 0 if m < 4 else 4
                    for k in range(KC):
                        nc.tensor.matmul(ps[:], w2[:, k, m * 128:(m + 1) * 128],
                                         qkr[:, base + k, :],
                                         start=(k == 0), stop=(k == KC - 1))
                    nc.vector.tensor_tensor(qk2[:, m, :], ps[:], cq[:, m, :],
                                            ALU.add)
                # attention heads (paired: exp(h+1) hides under A@V(h))
                o_t = a4.tile([128, KC, S], F16, tag="ot", bufs=2)
                for h0 in range(0, H, 2):
                    ats = {}
                    for hh in (h0, h0 + 1):
                        rows = slice(64 * (hh % 2), 64 * (hh % 2) + 64)
                        at = a4.tile([128, KC, S], F16, tag="at", bufs=3)
                        for c in range(KC):
                            scp = b1.tile([128, S], F32, tag="b1")
                            nc.tensor.matmul(
                                scp[:],
                                qk2[rows, 4 + hh // 2, c * 128:(c + 1) * 128],
                                qk2[rows, hh // 2, :], start=True, stop=True)
                            nc.scalar.activation(at[:, c, :], scp[:], AF.Exp,
                                                 scale=float(1.0 / np.sqrt(DH)))
                        ats[hh] = at
                    for hh in (h0, h0 + 1):
                        rows = slice(64 * (hh % 2), 64 * (hh % 2) + 64)
                        at = ats[hh]
                        ov = b2.tile([65, S], F32, tag="b2")
                        for c in range(KC):
                            nc.tensor.matmul(ov[:], vloc[:, c, hh, :], at[:, c, :],
                                             start=(c == 0), stop=(c == KC - 1))
                        # 1/denom = exp(-ln denom), broadcast via PE
                        lnd = scr.tile([1, S], F16, tag="lnd")
                        nc.scalar.activation(lnd[:], ov[64:65, :], AF.Ln)
                        rb = b2.tile([64, S], F32, tag="b2")
                        nc.tensor.matmul(rb[:], ones_row[:, 0:64], lnd[:],
                                         start=True, stop=True)
                        inv = invp.tile([64, S], F16, tag="inv")
                        nc.scalar.activation(inv[:], rb[:], AF.Exp, scale=-1.0)
                        nc.vector.tensor_tensor(o_t[rows, hh // 2, :],
                                                ov[0:64, :], inv[:], ALU.mult)
                # out-proj + residual
                for m in range(KC):
                    ps = b1.tile([128, S], F32, tag="b1")
                    for k in range(KC):
                        nc.tensor.matmul(ps[:], ow[:, k, m * 128:(m + 1) * 128],
                                         o_t[:, k, :], start=(k == 0),
                                         stop=(k == KC - 1))
                    nc.vector.scalar_tensor_tensor(
                        x[:, m, sl], ps[:], owb[:, m:m + 1], x[:, m, sl],
                        ALU.add, ALU.add)

            # ---- phase B: FFN (l2 streams in once w1 slot is released) ----
            l2 = wpool.tile([128, FC, D], F16, tag="bigw")
            for kf in range(FC):
                nc.sync.dma_start(l2[:, kf, :], dram["el2T"][li][:, kf, :])
            for s in range(BL):
                sl = slice(s * S, (s + 1) * S)
                rc2 = ln_stats(x, s)
                h2 = hp.tile([128, KC, S], F16, tag="h")
                ln_apply(rc2, x, s, h2)
                f2 = [b1.tile([128, S], F32, tag="b1", name=f"f2_{_m}")
                      for _m in range(KC)]
                for kf in range(FC):
                    f1 = b2.tile([128, S], F32, tag="b2")
                    for k in range(KC):
                        nc.tensor.matmul(f1[:], l1[:, k, kf * 128:(kf + 1) * 128],
                                         h2[:, k, :], start=(k == 0),
                                         stop=(k == KC - 1))
                    rt = rtp.tile([128, S], F16, tag="rt")
                    nc.scalar.activation(rt[:], f1[:], AF.Relu,
                                         bias=l1b[:, kf:kf + 1])
                    for m in range(KC):
                        nc.tensor.matmul(f2[m][:], l2[:, kf, m * 128:(m + 1) * 128],
                                         rt[:], start=(kf == 0), stop=(kf == FC - 1))
                for m in range(KC):
                    nc.vector.scalar_tensor_tensor(
                        x[:, m, sl], f2[m][:], l2b[:, m:m + 1], x[:, m, sl],
                        ALU.add, ALU.add)

        # ---------------- final encoder LN (in-place; affine folded) --------
        me = x
        for s in range(BL):
            rcf = ln_stats(x, s)
            ln_apply(rcf, x, s, x[:, :, s * S:(s + 1) * S])

        # ---------------- decoder layers ----------------
        for li in range(ld):
            dw = wpool.tile([128, KC, 3 * D], F16, tag="bigw")
            nc.sync.dma_start(dw[:], dram["dinT"][li])
            dwb = bpool.tile([128, 12], F32, tag="w1b")
            nc.sync.dma_start(dwb[:], dram["dinb"][li])
            dvbrow = bpool.tile([1, D], F16, tag="vbrow")
            nc.sync.dma_start(dvbrow[:], dram["dvb"][li])
            do = owpool.tile([128, KC, D], F16, tag="ow")
            nc.sync.dma_start(do[:], dram["dowT"][li])
            dob = bpool.tile([128, 4], F32, tag="owb")
            nc.sync.dma_start(dob[:], dram["dowb"][li])
            m1 = wpool.tile([128, KC, M], F16, tag="bigw")
            nc.sync.dma_start(m1[:], dram["dm1T"][li])
            m1b = bpool.tile([128, 16], F32, tag="l1b")
            nc.sync.dma_start(m1b[:], dram["dm1b"][li])
            m2b = bpool.tile([128, 4], F32, tag="l2b")
            nc.sync.dma_start(m2b[:], dram["dm2b"][li])

            vb_ps = b1.tile([128, D], F32, tag="b1")
            nc.tensor.matmul(vb_ps[:], ones_row[:], dvbrow[:], start=True, stop=True)
            vb_bc = vbp.tile([128, D], F16, tag="vb_bc")
            nc.scalar.activation(vb_bc[:], vb_ps[:], AF.Copy)

            # LN(p) -> q_ln ; Q projection (all samples at once, N=BL)
            q_ln = smalls.tile([128, KC, BL], F16, tag="q_ln")
            ln_small(p, BL, q_ln)
            qps = b1.tile([128, KC, BL], F32, tag="b1")
            for m in range(KC):
                for k in range(KC):
                    nc.tensor.matmul(qps[:, m, :],
                                     dw[:, k, m * 128:(m + 1) * 128],
                                     q_ln[:, k, :], start=(k == 0),
                                     stop=(k == KC - 1))
            q_sb = smalls.tile([128, KC, BL], F16, tag="q_sb")
            for m in range(KC):
                nc.scalar.activation(q_sb[:, m, :], qps[:, m, :], AF.Identity,
                                     bias=dwb[:, m:m + 1])
            o_d = smalls.tile([128, KC, BL], F16, tag="o_d")
            for s in range(BL):
                sl = slice(s * S, (s + 1) * S)
                # K (feature-major) and V' (token-major) over morph_enc
                k_sb = a4.tile([128, KC, S], F16, tag="at", bufs=3)
                for m in range(KC):
                    ps = b1.tile([128, S], F32, tag="b1")
                    for k in range(KC):
                        nc.tensor.matmul(
                            ps[:], dw[:, k, D + m * 128:D + (m + 1) * 128],
                            me[:, k, sl], start=(k == 0), stop=(k == KC - 1))
                    nc.scalar.activation(k_sb[:, m, :], ps[:], AF.Identity,
                                         bias=dwb[:, 4 + m:5 + m])
                vloc = vp.tile([128, KC, 8, 65], F16, tag="vloc")
                for t in range(KC):
                    nc.vector.tensor_copy(vloc[:, t, :, 64], ones8[:])
                for t in range(KC):
                    ps = b1.tile([128, S], F32, tag="b1")
                    for k in range(KC):
                        nc.tensor.matmul(
                            ps[:], me[:, k, s * S + t * 128:s * S + (t + 1) * 128],
                            dw[:, k, 2 * D:3 * D],
                            start=(k == 0), stop=(k == KC - 1))
                    nc.vector.tensor_tensor(
                        vloc[:, t, :, 0:64],
                        ps[:].rearrange("p (h d) -> p h d", h=H),
                        vb_bc[:].rearrange("p (h d) -> p h d", h=H), ALU.add)
                scp = b1.tile([128, KC, H], F32, tag="b1")
                for hh in range(H):
                    rows = slice(64 * (hh % 2), 64 * (hh % 2) + 64)
                    for c in range(KC):
                        nc.tensor.matmul(
                            scp[:, c, hh:hh + 1],
                            k_sb[rows, hh // 2, c * 128:(c + 1) * 128],
                            q_sb[rows, hh // 2, s:s + 1],
                            start=True, stop=True)
                at = smalls.tile([128, KC, H], F16, tag="at_d")
                nc.scalar.activation(at[:], scp[:], AF.Exp,
                                     scale=float(1.0 / np.sqrt(DH)))
                ov = b2.tile([65, H], F32, tag="b2")
                for hh in range(H):
                    for c in range(KC):
                        nc.tensor.matmul(ov[:, hh:hh + 1], vloc[:, c, hh, :],
                                         at[:, c, hh:hh + 1],
                                         start=(c == 0), stop=(c == KC - 1))
                # 1/denom = exp(-ln denom)
                lnd = scr.tile([1, H], F16, tag="lnd")
                nc.scalar.activation(lnd[:], ov[64:65, :], AF.Ln)
                rb = b2.tile([64, H], F32, tag="b2")
                nc.tensor.matmul(rb[:], ones_row[:, 0:64], lnd[:],
                                 start=True, stop=True)
                inv = invp.tile([64, H], F16, tag="inv_d")
                nc.scalar.activation(inv[:], rb[:], AF.Exp, scale=-1.0)
                for hh in range(H):
                    rows = slice(64 * (hh % 2), 64 * (hh % 2) + 64)
                    nc.vector.tensor_tensor(o_d[rows, hh // 2, s:s + 1],
                                            ov[0:64, hh:hh + 1],
                                            inv[:, hh:hh + 1], ALU.mult)
            # out-proj + residual into p
            ops = b1.tile([128, KC, BL], F32, tag="b1")
            for m in range(KC):
                for k in range(KC):
                    nc.tensor.matmul(ops[:, m, :],
                                     do[:, k, m * 128:(m + 1) * 128],
                                     o_d[:, k, :], start=(k == 0),
                                     stop=(k == KC - 1))
            for m in range(KC):
                nc.vector.scalar_tensor_tensor(
                    p[:, m, :], ops[:, m, :], dob[:, m:m + 1], p[:, m, :],
                    ALU.add, ALU.add)
            # FFN on p (m2 streams in chunked once dw releases its slot)
            m2 = wpool.tile([128, MC, D], F16, tag="bigw")
            for kf in range(MC):
                nc.sync.dma_start(m2[:, kf, :], dram["dm2T"][li][:, kf, :])
            h2d = smalls.tile([128, KC, BL], F16, tag="q_ln")
            ln_small(p, BL, h2d)
            mh = smalls.tile([128, MC, BL], F16, tag="mh")
            for mm_ in range(MC):
                ps = b1.tile([128, BL], F32, tag="b1")
                for k in range(KC):
                    nc.tensor.matmul(ps[:], m1[:, k, mm_ * 128:(mm_ + 1) * 128],
                                     h2d[:, k, :], start=(k == 0),
                                     stop=(k == KC - 1))
                nc.scalar.activation(mh[:, mm_, :], ps[:], AF.Relu,
                                     bias=m1b[:, mm_:mm_ + 1])
            m2ps = b1.tile([128, KC, BL], F32, tag="b1")
            for m in range(KC):
                for kf in range(MC):
                    nc.tensor.matmul(m2ps[:, m, :],
                                     m2[:, kf, m * 128:(m + 1) * 128],
                                     mh[:, kf, :], start=(kf == 0),
                                     stop=(kf == MC - 1))
            for m in range(KC):
                nc.vector.scalar_tensor_tensor(
                    p[:, m, :], m2ps[:, m, :], m2b[:, m:m + 1], p[:, m, :],
                    ALU.add, ALU.add)

        # ---------------- head ----------------
        hw = smalls.tile([128, KC], F16, tag="hw")
        hb = smalls.tile([1, 1], F32, tag="hb")
        nc.sync.dma_start(hw[:], dram["hwT"][:])
        nc.sync.dma_start(hb[:], dram["hb"][:])
        hg = smalls.tile([128, KC, BL], F16, tag="q_ln")
        ln_small(p, BL, hg)
        hps = b2.tile([1, BL], F32, tag="b2")
        for k in range(KC):
            nc.tensor.matmul(hps[:], hw[:, k:k + 1], hg[:, k, :],
                             start=(k == 0), stop=(k == KC - 1))
        y_sb = smalls.tile([1, BL], F32, tag="y_sb")
        nc.scalar.activation(y_sb[:], hps[:], AF.Sigmoid, bias=hb[:])
        nc.sync.dma_start(y_dram[:], y_sb[:])


# ----------------------------------------------------------------------------
# entry point
# ----------------------------------------------------------------------------

_NC_CACHE = {}


def kernel(**inputs):
    return _run(inputs, LE, LD)


def _run(inputs, le, ld, trace=False):
    w = prep_weights(inputs, le, ld)
    morph = np.asarray(inputs["morph"], np.float32)
    pose = np.asarray(inputs["pose"], np.float32)
    in_maps = []
    for c in range(NCORES):
        im = dict(w)
        mo = morph[c * BL:(c + 1) * BL]                 # [BL, S, 3]
        im["morphT"] = np.ascontiguousarray(
            mo.transpose(2, 0, 1).reshape(3, T)).astype(np.float16)
        im["poseT"] = np.ascontiguousarray(
            pose[c * BL:(c + 1) * BL].T).astype(np.float16)
        in_maps.append(im)

    if ("nc", le, ld) not in _NC_CACHE:
        _NC_CACHE[("nc", le, ld)] = build(le, ld)
    nc = _NC_CACHE[("nc", le, ld)]
    res = run_bass_kernel_spmd(nc, in_maps, core_ids=list(range(NCORES)),
                               trace=trace)
    out = np.zeros((B, 1), np.float32)
    for c in range(NCORES):
        out[c * BL:(c + 1) * BL, 0] = res.results[c]["y"][0]
    if trace:
        return out, res
    return out


# revision 17
# speedup vs baseline: 1.6792x; 1.2234x over previous
"""Trainium2 Bass kernel for nn_ReachabilityClassifierTransformer.

Data-parallel over batch: 16 samples / 8 cores = 2 samples per core.
Each core runs the full network (6-layer encoder + 4-layer decoder + head)
on its 2 samples. No collectives.

v2 (fp16 pipeline):
  - All matmul operands fp16 (stationary weights get Fast-Weight-Load; DVE
    elementwise ops run in 2x mode; DMA traffic halved). PSUM stays fp32.
  - V path folded on host: V = h @ (Wv2 Wv1)^T + bv  (no rope between the
    two V projections, so the double-projection quirk collapses).
  - Stage-1 q/k biases folded through rope into per-position bias tensors
    C_q/C_k = W2 @ rope(b1) + b2, added at the stage-2 PSUM copy (rope is
    linear, rotation depends only on position).
  - No Sqrt / no DVE reciprocal anywhere: LN rsqrt = exp(-0.5 ln(v+eps)),
    softmax 1/denom = exp(-ln denom) broadcast via PE.  ln/exp/copy/relu/
    square all live in one activation table set -> no table switches.
  - Per-layer phase order interleaves the two samples so rope (DVE) and
    softmax exp (Act) hide under the other sample's matmuls.

Device layout conventions (per core):
  - Activations FEATURE-MAJOR in SBUF: tile [128, KC, T] holds X.T.
  - Weights pre-transposed on host to [in_feat, out_feat], laid out
    [128, KC_in, O] (partition = in-feature % 128).
  - matmul(out_psum[M,N], lhsT=[K,M], rhs=[K,N]) computes lhsT.T @ rhs.
  - Encoder stage-1 q,k output features are de-interleaved (even feats then
    odd feats) via host-side column permutation, so RoPE becomes contiguous
    block ops; the roped result is in natural (concatenated) order.
  - Softmax: scores computed transposed (S.T = K_h @ Q_h.T per chunk),
    exp'd without max subtraction (|scores/8| < 1 for this model), and the
    denominator comes free from a ones-column appended to V.
"""
import functools

import numpy as np

import concourse.bass as bass
import concourse.mybir as mybir
import concourse.tile as tile
from concourse import bacc
from concourse.bass_utils import run_bass_kernel_spmd


def _patch_act_tables():
    """Constrain exp/ln to the one table set that contains both.

    The act-table-load pass maps each activation function to a set
    independently (exp -> exp_and_others, ln -> natural_log), so a kernel
    that interleaves exp and ln reloads tables on every transition
    (~1.3us each).  natural_log_exp_and_others contains exp AND ln (plus
    copy/identity/relu/square), so restricting exp/ln to that set makes
    every load resolve there; set ids/order are preserved so the emitted
    act_func_set_id still indexes the real act_info.json.
    """
    import concourse.hw_specs as hw_specs
    if getattr(hw_specs, "_ant_act_tables_patched", False):
        return
    orig = hw_specs.get_activation_tables

    @functools.cache
    def patched(module_arch):
        t = orig(module_arch)
        keep = "natural_log_exp_and_others"
        if keep not in t:
            return t
        drop = {mybir.ActivationFunctionType.Exp, mybir.ActivationFunctionType.Ln}
        return {name: (fns if name == keep else fns - drop)
                for name, fns in t.items()}

    hw_specs._ant_act_tables_patched = True
    hw_specs.get_activation_tables = patched
    import sys
    for modname in ("concourse.bacc", "concourse.bass_interp"):
        mod = sys.modules.get(modname)
        if mod is not None and hasattr(mod, "get_activation_tables"):
            mod.get_activation_tables = patched


_patch_act_tables()

AF = mybir.ActivationFunctionType
ALU = mybir.AluOpType
F32 = mybir.dt.float32
F16 = mybir.dt.float16

B, S, D, FF, H, LE, LD, M = 16, 512, 512, 2048, 8, 6, 4, 2048
ROPE_BASE = 10000.0
LN_EPS = 1e-5
NCORES = 8
BL = B // NCORES          # 2 samples per core
T = BL * S                # 1024 tokens per core
KC = D // 128             # 4 feature chunks
FC = FF // 128            # 16
MC = M // 128             # 16
DH = D // H               # 64


# ----------------------------------------------------------------------------
# host-side helpers
# ----------------------------------------------------------------------------

def _chunked(wT):
    """[Din, O] -> [128, Din//128, O] contiguous fp16."""
    Din, O = wT.shape
    return np.ascontiguousarray(
        wT.reshape(Din // 128, 128, O).transpose(1, 0, 2)).astype(np.float16)


def _bias_cols(b):
    """[O] -> [128, O//128]  (column per 128-chunk), fp32."""
    O = b.shape[0]
    return np.ascontiguousarray(b.reshape(O // 128, 128).T).astype(np.float32)


_DEINT = np.concatenate([np.arange(0, D, 2), np.arange(1, D, 2)])  # de-interleave


def prep_weights(inp, le=LE, ld=LD):
    """Host-side weight prep -> dict of arrays shared by all cores."""
    out = {}
    g = {k: np.asarray(v, np.float64) for k, v in inp.items()}

    out["mpwT"] = np.ascontiguousarray(g["morph_proj_w"].T).astype(np.float16)
    out["mpb"] = _bias_cols(g["morph_proj_b"])                     # [128, 4]
    out["ppwT"] = np.ascontiguousarray(g["pose_proj_w"].T).astype(np.float16)
    out["ppb"] = _bias_cols(g["pose_proj_b"])

    # rope grids, de-interleaved frequency order: [128, 2, 512] fp16
    freq = 1.0 / ROPE_BASE ** (np.arange(0, D, 2, dtype=np.float64) / D)
    ang = np.outer(np.arange(S, dtype=np.float64), freq)           # [512, 256]
    cosT = np.cos(ang).T                                           # [256, S]
    sinT = np.sin(ang).T
    out["gridc"] = _chunked(cosT.reshape(256, S))
    out["grids"] = _chunked(sinT.reshape(256, S))

    e_w1, e_w2, e_wv, e_cqk, e_vb = [], [], [], [], []
    e_ow, e_owb, e_l1, e_l1b, e_l2, e_l2b = [], [], [], [], [], []
    for i in range(le):
        w1 = g["enc_in_w"][i] * g["enc_n1_g"][i][None, :]          # fold n1 g
        b1 = g["enc_in_b"][i] + g["enc_in_w"][i] @ g["enc_n1_b"][i]
        # stage-1 q,k only, de-interleaved output columns
        perm = np.concatenate([_DEINT, D + _DEINT])
        e_w1.append(_chunked(np.ascontiguousarray(w1[perm].T)))    # [128,4,1024]
        # stage-2 q,k (natural order, raw weights - the faithful quirk)
        w2 = g["enc_in_w"][i][: 2 * D]                             # Wq;Wk
        e_w2.append(_chunked(np.ascontiguousarray(w2.T)))          # [128,4,1024]
        # stage-2 bias tensors: C = W2 @ rope(b1) + b2   [512, S] each
        bq = b1[:D][_DEINT]                                        # [even; odd]
        bk = b1[D:2 * D][_DEINT]
        rb_q = np.concatenate([bq[:256, None] * cosT - bq[256:, None] * sinT,
                               bq[:256, None] * sinT + bq[256:, None] * cosT])
        rb_k = np.concatenate([bk[:256, None] * cosT - bk[256:, None] * sinT,
                               bk[:256, None] * sinT + bk[256:, None] * cosT])
        Cq = g["enc_in_w"][i][:D] @ rb_q + g["enc_in_b"][i][:D][:, None]
        Ck = g["enc_in_w"][i][D:2 * D] @ rb_k \
            + g["enc_in_b"][i][D:2 * D][:, None]
        C = np.concatenate([Cq, Ck], axis=0)                       # [1024, S]
        e_cqk.append(_chunked(C))                                  # [128,8,S]
        # V folded: V = h @ (Wv2 Wv1_f).T + (Wv2 bv1_f + bv2)
        Wv1f = w1[2 * D:]
        bv1f = b1[2 * D:]
        Wv2 = g["enc_in_w"][i][2 * D:]
        bv2 = g["enc_in_b"][i][2 * D:]
        e_wv.append(_chunked(np.ascontiguousarray((Wv2 @ Wv1f).T)))
        e_vb.append((Wv2 @ bv1f + bv2)[None, :].astype(np.float16))  # [1,512]
        e_ow.append(_chunked(np.ascontiguousarray(g["enc_out_w"][i].T)))
        e_owb.append(_bias_cols(g["enc_out_b"][i]))
        l1 = g["enc_l1_w"][i] * g["enc_n2_g"][i][None, :]
        l1b = g["enc_l1_b"][i] + g["enc_l1_w"][i] @ g["enc_n2_b"][i]
        e_l1.append(_chunked(np.ascontiguousarray(l1.T)))          # [128,4,2048]
        e_l1b.append(_bias_cols(l1b))                              # [128,16]
        e_l2.append(_chunked(np.ascontiguousarray(g["enc_l2_w"][i].T)))
        e_l2b.append(_bias_cols(g["enc_l2_b"][i]))                 # [128,4]
    out["ew1T"] = np.stack(e_w1) if le else np.zeros((0, 128, KC, 2 * D), np.float16)
    out["ew2T"] = np.stack(e_w2) if le else np.zeros((0, 128, KC, 2 * D), np.float16)
    out["ewvT"] = np.stack(e_wv) if le else np.zeros((0, 128, KC, D), np.float16)
    out["ecqk"] = np.stack(e_cqk) if le else np.zeros((0, 128, 8, S), np.float16)
    out["evb"] = np.stack(e_vb) if le else np.zeros((0, 1, D), np.float16)
    out["eowT"] = np.stack(e_ow) if le else np.zeros((0, 128, KC, D), np.float16)
    out["eowb"] = np.stack(e_owb) if le else np.zeros((0, 128, 4), np.float32)
    out["el1T"] = np.stack(e_l1) if le else np.zeros((0, 128, KC, FF), np.float16)
    out["el1b"] = np.stack(e_l1b) if le else np.zeros((0, 128, 16), np.float32)
    out["el2T"] = np.stack(e_l2) if le else np.zeros((0, 128, FC, D), np.float16)
    out["el2b"] = np.stack(e_l2b) if le else np.zeros((0, 128, 4), np.float32)

    d_in, d_inb, d_vb, d_ow, d_owb = [], [], [], [], []
    d_m1, d_m1b, d_m2, d_m2b = [], [], [], []
    for i in range(ld):
        w = g["dec_in_w"][i].copy()
        b = g["dec_in_b"][i].copy()
        w[:D] = w[:D] * g["dec_n1_g"][i][None, :]                  # Wq <- dec_n1
        b[:D] = b[:D] + g["dec_in_w"][i][:D] @ g["dec_n1_b"][i]
        w[D:] = w[D:] * g["enc_final_g"][None, :]                  # Wk,Wv <- enc_final
        b[D:] = b[D:] + g["dec_in_w"][i][D:] @ g["enc_final_b"]
        d_in.append(_chunked(np.ascontiguousarray(w.T)))           # [128,4,1536]
        d_inb.append(_bias_cols(b))
        d_vb.append(b[2 * D:][None, :].astype(np.float16))         # [1,512]
        d_ow.append(_chunked(np.ascontiguousarray(g["dec_out_w"][i].T)))
        d_owb.append(_bias_cols(g["dec_out_b"][i]))
        m1 = g["dec_m1_w"][i] * g["dec_n2_g"][i][None, :]
        m1b = g["dec_m1_b"][i] + g["dec_m1_w"][i] @ g["dec_n2_b"][i]
        d_m1.append(_chunked(np.ascontiguousarray(m1.T)))          # [128,4,2048]
        d_m1b.append(_bias_cols(m1b))
        d_m2.append(_chunked(np.ascontiguousarray(g["dec_m2_w"][i].T)))
        d_m2b.append(_bias_cols(g["dec_m2_b"][i]))
    out["dinT"] = np.stack(d_in) if ld else np.zeros((0, 128, KC, 3 * D), np.float16)
    out["dinb"] = np.stack(d_inb) if ld else np.zeros((0, 128, 12), np.float32)
    out["dvb"] = np.stack(d_vb) if ld else np.zeros((0, 1, D), np.float16)
    out["dowT"] = np.stack(d_ow) if ld else np.zeros((0, 128, KC, D), np.float16)
    out["dowb"] = np.stack(d_owb) if ld else np.zeros((0, 128, 4), np.float32)
    out["dm1T"] = np.stack(d_m1) if ld else np.zeros((0, 128, KC, M), np.float16)
    out["dm1b"] = np.stack(d_m1b) if ld else np.zeros((0, 128, 16), np.float32)
    out["dm2T"] = np.stack(d_m2) if ld else np.zeros((0, 128, MC, D), np.float16)
    out["dm2b"] = np.stack(d_m2b) if ld else np.zeros((0, 128, 4), np.float32)

    hw = (g["head_w"] * g["head_g"][None, :])[0]                   # [512]
    out["hwT"] = _bias_cols(hw).astype(np.float16)                 # [128, 4]
    out["hb"] = (g["head_bias"] + g["head_w"] @ g["head_b"]).reshape(1, 1).astype(np.float32)
    return out


# ----------------------------------------------------------------------------
# device program
# ----------------------------------------------------------------------------

def build(le=LE, ld=LD):
    nc = bacc.Bacc(None, target_bir_lowering=False)

    dram = {}

    def din(name, shape, dt=F16):
        dram[name] = nc.dram_tensor(name, list(shape), dt, kind="ExternalInput")
        return dram[name]

    # shared weights
    din("mpwT", [3, 512]); din("mpb", [128, 4], F32)
    din("ppwT", [9, 512]); din("ppb", [128, 4], F32)
    din("gridc", [128, 2, S]); din("grids", [128, 2, S])
    din("ew1T", [le, 128, KC, 2 * D])
    din("ew2T", [le, 128, KC, 2 * D])
    din("ewvT", [le, 128, KC, D])
    din("ecqk", [le, 128, 8, S])
    din("evb", [le, 1, D])
    din("eowT", [le, 128, KC, D]); din("eowb", [le, 128, 4], F32)
    din("el1T", [le, 128, KC, FF]); din("el1b", [le, 128, 16], F32)
    din("el2T", [le, 128, FC, D]); din("el2b", [le, 128, 4], F32)
    din("dinT", [ld, 128, KC, 3 * D]); din("dinb", [ld, 128, 12], F32)
    din("dvb", [ld, 1, D])
    din("dowT", [ld, 128, KC, D]); din("dowb", [ld, 128, 4], F32)
    din("dm1T", [ld, 128, KC, M]); din("dm1b", [ld, 128, 16], F32)
    din("dm2T", [ld, 128, MC, D]); din("dm2b", [ld, 128, 4], F32)
    din("hwT", [128, KC]); din("hb", [1, 1], F32)
    # per-core inputs
    din("morphT", [3, T])
    din("poseT", [9, BL])
    y = nc.dram_tensor("y", [1, BL], F32, kind="ExternalOutput")

    with tile.TileContext(nc) as tc:
        _build_body(nc, tc, dram, y, le, ld)
    nc.compile()
    return nc


def _build_body(nc, tc, dram, y_dram, le, ld):
    import contextlib
    ctx = contextlib.ExitStack()
    with ctx:
        ctx.enter_context(nc.allow_low_precision(
            reason="fp16 matmul operands / activations are intentional"))
        persist = ctx.enter_context(tc.tile_pool(name="persist", bufs=1))
        wpool = ctx.enter_context(tc.tile_pool(name="wpool", bufs=2))
        w2pool = ctx.enter_context(tc.tile_pool(name="w2pool", bufs=1))
        owpool = ctx.enter_context(tc.tile_pool(name="owpool", bufs=1))
        vwpool = ctx.enter_context(tc.tile_pool(name="vwpool", bufs=1))
        cqpool = ctx.enter_context(tc.tile_pool(name="cqpool", bufs=1))
        bpool = ctx.enter_context(tc.tile_pool(name="bpool", bufs=2))
        a4 = ctx.enter_context(tc.tile_pool(name="a4", bufs=4))
        hp = ctx.enter_context(tc.tile_pool(name="hp", bufs=2))
        a8 = ctx.enter_context(tc.tile_pool(name="a8", bufs=3))
        qk2p = ctx.enter_context(tc.tile_pool(name="qk2p", bufs=2))
        vp = ctx.enter_context(tc.tile_pool(name="vp", bufs=2))
        rtp = ctx.enter_context(tc.tile_pool(name="rtp", bufs=3))
        vbp = ctx.enter_context(tc.tile_pool(name="vbp", bufs=1))
        scr = ctx.enter_context(tc.tile_pool(name="scr", bufs=3))
        invp = ctx.enter_context(tc.tile_pool(name="invp", bufs=2))
        rcp = ctx.enter_context(tc.tile_pool(name="rcp", bufs=2))
        smalls = ctx.enter_context(tc.tile_pool(name="smalls", bufs=2))
        b1 = ctx.enter_context(tc.tile_pool(name="b1", bufs=4, space="PSUM"))
        b2 = ctx.enter_context(tc.tile_pool(name="b2", bufs=4, space="PSUM"))

        # ---------------- persistent tiles ----------------
        x = persist.tile([128, KC, T], F16)           # residual stream (X.T)
        gridc = persist.tile([128, 2, S], F16)
        grids = persist.tile([128, 2, S], F16)
        ones128 = persist.tile([128, 1], F16)
        ones_row = persist.tile([1, 128], F16)
        ones8 = persist.tile([128, 8], F16)
        eps_t = persist.tile([1, 1], F32)
        p = persist.tile([128, KC, BL], F16)          # decoder latent p.T
        nc.sync.dma_start(gridc[:], dram["gridc"][:])
        nc.sync.dma_start(grids[:], dram["grids"][:])
        stage_f16 = rtp.tile([128, 128], F16, tag="rt")
        nc.vector.memset(stage_f16[:], 1.0)
        nc.vector.tensor_copy(ones128[:], stage_f16[:, 0:1])
        nc.vector.tensor_copy(ones_row[:], stage_f16[0:1, :])
        nc.vector.tensor_copy(ones8[:], stage_f16[:, 0:8])
        nc.vector.memset(eps_t[:], LN_EPS)

        def ln_small(x_tile, n_tok, h_out):
            """h_out = LayerNorm_features(x_tile) for tiny n_tok (decoder)."""
            sq = smalls.tile([128, KC, n_tok], F16, tag="sq_d")
            for k in range(KC):
                nc.vector.tensor_tensor(sq[:, k, :], x_tile[:, k, :],
                                        x_tile[:, k, :], ALU.mult)
            sum_ps = b2.tile([1, n_tok], F32, tag="b2")
            sq_ps = b2.tile([1, n_tok], F32, tag="b2")
            for k in range(KC):
                nc.tensor.matmul(sum_ps[:], ones128[:], x_tile[:, k, :],
                                 start=(k == 0), stop=(k == KC - 1))
            for k in range(KC):
                nc.tensor.matmul(sq_ps[:], ones128[:], sq[:, k, :],
                                 start=(k == 0), stop=(k == KC - 1))
            ms = scr.tile([1, n_tok], F32, tag="scr")
            t2 = scr.tile([1, n_tok], F32, tag="scr")
            rc = scr.tile([1, 2, n_tok], F16, tag="scr_rc")
            nc.scalar.activation(ms[:], sum_ps[:], AF.Copy, scale=1.0 / D)
            nc.vector.tensor_tensor(t2[:], ms[:], ms[:], ALU.mult)      # m^2
            nc.vector.scalar_tensor_tensor(
                t2[:], sq_ps[:], 1.0 / D, t2[:], ALU.mult, ALU.subtract)
            # r = exp(-0.5 ln(var + eps))
            nc.scalar.activation(t2[:], t2[:], AF.Ln, bias=eps_t[:])
            nc.scalar.activation(rc[:, 0, :], t2[:], AF.Exp, scale=-0.5)
            nc.vector.scalar_tensor_tensor(
                rc[:, 1, :], ms[:], -1.0, rc[:, 0, :], ALU.mult, ALU.mult)
            r_ps = b1.tile([128, n_tok], F32, tag="b1")
            c_ps = b1.tile([128, n_tok], F32, tag="b1")
            nc.tensor.matmul(r_ps[:], ones_row[:], rc[:, 0, :],
                             start=True, stop=True)
            nc.tensor.matmul(c_ps[:], ones_row[:], rc[:, 1, :],
                             start=True, stop=True)
            for k in range(KC):
                nc.vector.tensor_tensor(h_out[:, k, :], x_tile[:, k, :],
                                        r_ps[:], ALU.mult)
                nc.vector.tensor_tensor(h_out[:, k, :], h_out[:, k, :],
                                        c_ps[:], ALU.add)

        def ln_stats(x_tile, s):
            """Per-sample LN stats -> rc [1, 2, S] fp16 (r, c)."""
            sl = slice(s * S, (s + 1) * S)
            sq = a4.tile([128, KC, S], F16, tag="sq", bufs=2)
            nc.vector.tensor_tensor(sq[:], x_tile[:, :, sl], x_tile[:, :, sl],
                                    ALU.mult)
            sum_ps = b2.tile([1, S], F32, tag="b2")
            sq_ps = b2.tile([1, S], F32, tag="b2")
            for k in range(KC):
                nc.tensor.matmul(sum_ps[:], ones128[:], x_tile[:, k, sl],
                                 start=(k == 0), stop=(k == KC - 1))
            for k in range(KC):
                nc.tensor.matmul(sq_ps[:], ones128[:], sq[:, k, :],
                                 start=(k == 0), stop=(k == KC - 1))
            ms = scr.tile([1, S], F32, tag="scr")
            t2 = scr.tile([1, S], F32, tag="scr")
            rc = rcp.tile([1, 2, S], F16, tag="rc", bufs=4)
            nc.scalar.activation(ms[:], sum_ps[:], AF.Copy, scale=1.0 / D)
            nc.vector.tensor_tensor(t2[:], ms[:], ms[:], ALU.mult)
            nc.vector.scalar_tensor_tensor(
                t2[:], sq_ps[:], 1.0 / D, t2[:], ALU.mult, ALU.subtract)
            nc.scalar.activation(t2[:], t2[:], AF.Ln, bias=eps_t[:])
            nc.scalar.activation(rc[:, 0, :], t2[:], AF.Exp, scale=-0.5)
            nc.vector.scalar_tensor_tensor(
                rc[:, 1, :], ms[:], -1.0, rc[:, 0, :], ALU.mult, ALU.mult)
            return rc

        def ln_apply(rc, x_tile, s, h_out):
            """h_out[:, :, :] = x[:, :, s] * r + c  (r,c broadcast via PE)."""
            sl = slice(s * S, (s + 1) * S)
            r_ps = b1.tile([128, S], F32, tag="b1")
            c_ps = b1.tile([128, S], F32, tag="b1")
            nc.tensor.matmul(r_ps[:], ones_row[:], rc[:, 0, :],
                             start=True, stop=True)
            nc.tensor.matmul(c_ps[:], ones_row[:], rc[:, 1, :],
                             start=True, stop=True)
            rcb = rcp.tile([128, 2, S], F16, tag="rcb")
            nc.scalar.activation(rcb[:, 0, :], r_ps[:], AF.Copy)
            nc.scalar.activation(rcb[:, 1, :], c_ps[:], AF.Copy)
            for k in range(KC):
                nc.vector.tensor_tensor(h_out[:, k, :], x_tile[:, k, sl],
                                        rcb[:, 0, :], ALU.mult)
                nc.vector.tensor_tensor(h_out[:, k, :], h_out[:, k, :],
                                        rcb[:, 1, :], ALU.add)

        # ---------------- morph projection -> x ----------------
        morpht = rtp.tile([3, T], F16, tag="morph", bufs=1)
        nc.sync.dma_start(morpht[:], dram["morphT"][:])
        mpw = rtp.tile([3, 512], F16, tag="rt")
        mpb = rtp.tile([128, 4], F32, tag="rtb")
        nc.sync.dma_start(mpw[:], dram["mpwT"][:])
        nc.sync.dma_start(mpb[:], dram["mpb"][:])
        rc1 = {}
        for s in range(BL):
            for m in range(KC):
                ps = b1.tile([128, S], F32, tag="b1")
                nc.tensor.matmul(ps[:], mpw[:, m * 128:(m + 1) * 128],
                                 morpht[:, s * S:(s + 1) * S], start=True, stop=True)
                nc.scalar.activation(x[:, m, s * S:(s + 1) * S], ps[:], AF.Relu,
                                     bias=mpb[:, m:m + 1])
            rc1[s] = ln_stats(x, s)

        # ---------------- pose projection -> p ----------------
        poset = rtp.tile([9, BL], F16, tag="rt")
        ppw = rtp.tile([9, 512], F16, tag="rt")
        ppb = rtp.tile([128, 4], F32, tag="rtb")
        nc.sync.dma_start(poset[:], dram["poseT"][:])
        nc.sync.dma_start(ppw[:], dram["ppwT"][:])
        nc.sync.dma_start(ppb[:], dram["ppb"][:])
        pps = b1.tile([128, KC, BL], F32, tag="b1")
        for m in range(KC):
            nc.tensor.matmul(pps[:, m, :], ppw[:, m * 128:(m + 1) * 128],
                             poset[:], start=True, stop=True)
        for m in range(KC):
            nc.scalar.activation(p[:, m, :], pps[:, m, :], AF.Relu,
                                 bias=ppb[:, m:m + 1])

        # ---------------- encoder layers ----------------
        for li in range(le):
            w1 = wpool.tile([128, KC, 2 * D], F16, tag="bigw")
            nc.sync.dma_start(w1[:], dram["ew1T"][li])
            w2 = w2pool.tile([128, KC, 2 * D], F16, tag="w2")
            nc.sync.dma_start(w2[:], dram["ew2T"][li])
            wv = vwpool.tile([128, KC, D], F16, tag="wv")
            nc.sync.dma_start(wv[:], dram["ewvT"][li])
            cq = cqpool.tile([128, 8, S], F16, tag="cq")
            nc.sync.dma_start(cq[:], dram["ecqk"][li])
            vbrow = bpool.tile([1, D], F16, tag="vbrow")
            nc.sync.dma_start(vbrow[:], dram["evb"][li])
            ow = owpool.tile([128, KC, D], F16, tag="ow")
            nc.sync.dma_start(ow[:], dram["eowT"][li])
            owb = bpool.tile([128, 4], F32, tag="owb")
            nc.sync.dma_start(owb[:], dram["eowb"][li])
            l1 = wpool.tile([128, KC, FF], F16, tag="bigw")
            nc.sync.dma_start(l1[:], dram["el1T"][li])
            l1b = bpool.tile([128, 16], F32, tag="l1b")
            nc.sync.dma_start(l1b[:], dram["el1b"][li])
            l2b = bpool.tile([128, 4], F32, tag="l2b")
            nc.sync.dma_start(l2b[:], dram["el2b"][li])

            # v-bias broadcast [128, 512], once per layer
            vb_ps = b1.tile([128, D], F32, tag="b1")
            nc.tensor.matmul(vb_ps[:], ones_row[:], vbrow[:], start=True, stop=True)
            vb_bc = vbp.tile([128, D], F16, tag="vb_bc")
            nc.scalar.activation(vb_bc[:], vb_ps[:], AF.Copy)

            qkrs, vlocs = {}, {}
            # ---- phase A1 per sample: LN1-apply, stage1 q,k; V; rope ----
            # (rc1 stats were emitted at the end of the previous phase so the
            #  scalar chain overlapped that phase's matmuls)
            for s in range(BL):
                h = hp.tile([128, KC, S], F16, tag="h")
                ln_apply(rc1[s], x, s, h)
                qkv1 = a8.tile([128, 8, S], F16, tag="a8")
                for m in range(8):
                    ps = b1.tile([128, S], F32, tag="b1")
                    for k in range(KC):
                        nc.tensor.matmul(ps[:], w1[:, k, m * 128:(m + 1) * 128],
                                         h[:, k, :], start=(k == 0), stop=(k == KC - 1))
                    nc.scalar.activation(qkv1[:, m, :], ps[:], AF.Copy)
                # V token-major (+ones col), from h directly (folded weights)
                vloc = vp.tile([128, KC, 8, 65], F16, tag="vloc")
                for t in range(KC):
                    nc.vector.tensor_copy(vloc[:, t, :, 64], ones8[:])
                for t in range(KC):
                    ps = b1.tile([128, S], F32, tag="b1")
                    for k in range(KC):
                        nc.tensor.matmul(
                            ps[:], h[:, k, t * 128:(t + 1) * 128],
                            wv[:, k, :], start=(k == 0), stop=(k == KC - 1))
                    nc.vector.tensor_tensor(
                        vloc[:, t, :, 0:64],
                        ps[:].rearrange("p (h d) -> p h d", h=H),
                        vb_bc[:].rearrange("p (h d) -> p h d", h=H), ALU.add)
                vlocs[s] = vloc
                # rope: qkv1 (de-interleaved) -> qkr (natural order), DVE fp16
                qkr = a8.tile([128, 8, S], F16, tag="a8")
                for half in (0, 4):
                    for c in range(2):
                        e = qkv1[:, half + c, :]
                        o = qkv1[:, half + 2 + c, :]
                        r1 = qkr[:, half + c, :]
                        r2 = qkr[:, half + 2 + c, :]
                        t1 = rtp.tile([128, S], F16, tag="rt")
                        nc.vector.tensor_tensor(r1, e, gridc[:, c, :], ALU.mult)
                        nc.vector.tensor_tensor(t1[:], o, grids[:, c, :], ALU.mult)
                        nc.vector.tensor_tensor(r1, r1, t1[:], ALU.subtract)
                        t2 = rtp.tile([128, S], F16, tag="rt")
                        nc.vector.tensor_tensor(r2, e, grids[:, c, :], ALU.mult)
                        nc.vector.tensor_tensor(t2[:], o, gridc[:, c, :], ALU.mult)
                        nc.vector.tensor_tensor(r2, r2, t2[:], ALU.add)
                qkrs[s] = qkr

            # ---- phase A2 per sample: stage2, attention, out-proj ----
            rc2 = {}
            for s in range(BL):
                sl = slice(s * S, (s + 1) * S)
                qkr, vloc = qkrs[s], vlocs[s]
                qk2 = qk2p.tile([128, 8, S], F16, tag="qk2")
                for m in range(8):
                    ps = b1.tile([128, S], F32, tag="b1")
                    base = 0 if m < 4 else 4
                    for k in range(KC):
                        nc.tensor.matmul(ps[:], w2[:, k, m * 128:(m + 1) * 128],
                                         qkr[:, base + k, :],
                                         start=(k == 0), stop=(k == KC - 1))
                    nc.vector.tensor_tensor(qk2[:, m, :], ps[:], cq[:, m, :],
                                            ALU.add)
                # attention heads (paired: exp(h+1) hides under A@V(h))
                o_t = a4.tile([128, KC, S], F16, tag="ot", bufs=2)
                for h0 in range(0, H, 2):
                    ats = {}
                    for hh in (h0, h0 + 1):
                        rows = slice(64 * (hh % 2), 64 * (hh % 2) + 64)
                        at = a4.tile([128, KC, S], F16, tag="at", bufs=3)
                        for c in range(KC):
                            scp = b1.tile([128, S], F32, tag="b1")
                            nc.tensor.matmul(
                                scp[:],
                                qk2[rows, 4 + hh // 2, c * 128:(c + 1) * 128],
                                qk2[rows, hh // 2, :], start=True, stop=True)
                            nc.scalar.activation(at[:, c, :], scp[:], AF.Exp,
                                                 scale=float(1.0 / np.sqrt(DH)))
                        ats[hh] = at
                    for hh in (h0, h0 + 1):
                        rows = slice(64 * (hh % 2), 64 * (hh % 2) + 64)
                        at = ats[hh]
                        ov = b2.tile([65, S], F32, tag="b2")
                        for c in range(KC):
                            nc.tensor.matmul(ov[:], vloc[:, c, hh, :], at[:, c, :],
                                             start=(c == 0), stop=(c == KC - 1))
                        # 1/denom = exp(-ln denom), broadcast via PE
                        lnd = scr.tile([1, S], F16, tag="lnd")
                        nc.scalar.activation(lnd[:], ov[64:65, :], AF.Ln)
                        rb = b2.tile([64, S], F32, tag="b2")
                        nc.tensor.matmul(rb[:], ones_row[:, 0:64], lnd[:],
                                         start=True, stop=True)
                        inv = invp.tile([64, S], F16, tag="inv")
                        nc.scalar.activation(inv[:], rb[:], AF.Exp, scale=-1.0)
                        nc.vector.tensor_tensor(o_t[rows, hh // 2, :],
                                                ov[0:64, :], inv[:], ALU.mult)
                # out-proj + residual
                for m in range(KC):
                    ps = b1.tile([128, S], F32, tag="b1")
                    for k in range(KC):
                        nc.tensor.matmul(ps[:], ow[:, k, m * 128:(m + 1) * 128],
                                         o_t[:, k, :], start=(k == 0),
                                         stop=(k == KC - 1))
                    nc.vector.scalar_tensor_tensor(
                        x[:, m, sl], ps[:], owb[:, m:m + 1], x[:, m, sl],
                        ALU.add, ALU.add)
                rc2[s] = ln_stats(x, s)

            # ---- phase B: FFN (l2 streams in once w1 slot is released) ----
            l2 = wpool.tile([128, FC, D], F16, tag="bigw")
            for kf in range(FC):
                nc.sync.dma_start(l2[:, kf, :], dram["el2T"][li][:, kf, :])
            for s in range(BL):
                sl = slice(s * S, (s + 1) * S)
                h2 = hp.tile([128, KC, S], F16, tag="h")
                ln_apply(rc2[s], x, s, h2)
                f2 = [b1.tile([128, S], F32, tag="b1", name=f"f2_{_m}")
                      for _m in range(KC)]
                for kf in range(FC):
                    f1 = b2.tile([128, S], F32, tag="b2")
                    for k in range(KC):
                        nc.tensor.matmul(f1[:], l1[:, k, kf * 128:(kf + 1) * 128],
                                         h2[:, k, :], start=(k == 0),
                                         stop=(k == KC - 1))
                    rt = rtp.tile([128, S], F16, tag="rt")
                    nc.scalar.activation(rt[:], f1[:], AF.Relu,
                                         bias=l1b[:, kf:kf + 1])
                    for m in range(KC):
                        nc.tensor.matmul(f2[m][:], l2[:, kf, m * 128:(m + 1) * 128],
                                         rt[:], start=(kf == 0), stop=(kf == FC - 1))
                for m in range(KC):
                    nc.vector.scalar_tensor_tensor(
                        x[:, m, sl], f2[m][:], l2b[:, m:m + 1], x[:, m, sl],
                        ALU.add, ALU.add)
                rc1[s] = ln_stats(x, s)   # next layer (or the final LN)

        # ---------------- final encoder LN (in-place; affine folded) --------
        me = x
        for s in range(BL):
            ln_apply(rc1[s], x, s, x[:, :, s * S:(s + 1) * S])

        # ---------------- decoder layers ----------------
        for li in range(ld):
            dw = wpool.tile([128, KC, 3 * D], F16, tag="bigw")
            nc.sync.dma_start(dw[:], dram["dinT"][li])
            dwb = bpool.tile([128, 12], F32, tag="w1b")
            nc.sync.dma_start(dwb[:], dram["dinb"][li])
            dvbrow = bpool.tile([1, D], F16, tag="vbrow")
            nc.sync.dma_start(dvbrow[:], dram["dvb"][li])
            do = owpool.tile([128, KC, D], F16, tag="ow")
            nc.sync.dma_start(do[:], dram["dowT"][li])
            dob = bpool.tile([128, 4], F32, tag="owb")
            nc.sync.dma_start(dob[:], dram["dowb"][li])
            m1 = wpool.tile([128, KC, M], F16, tag="bigw")
            nc.sync.dma_start(m1[:], dram["dm1T"][li])
            m1b = bpool.tile([128, 16], F32, tag="l1b")
            nc.sync.dma_start(m1b[:], dram["dm1b"][li])
            m2b = bpool.tile([128, 4], F32, tag="l2b")
            nc.sync.dma_start(m2b[:], dram["dm2b"][li])

            vb_ps = b1.tile([128, D], F32, tag="b1")
            nc.tensor.matmul(vb_ps[:], ones_row[:], dvbrow[:], start=True, stop=True)
            vb_bc = vbp.tile([128, D], F16, tag="vb_bc")
            nc.scalar.activation(vb_bc[:], vb_ps[:], AF.Copy)

            # LN(p) -> q_ln ; Q projection (all samples at once, N=BL)
            q_ln = smalls.tile([128, KC, BL], F16, tag="q_ln")
            ln_small(p, BL, q_ln)
            qps = b1.tile([128, KC, BL], F32, tag="b1")
            for m in range(KC):
                for k in range(KC):
                    nc.tensor.matmul(qps[:, m, :],
                                     dw[:, k, m * 128:(m + 1) * 128],
                                     q_ln[:, k, :], start=(k == 0),
                                     stop=(k == KC - 1))
            q_sb = smalls.tile([128, KC, BL], F16, tag="q_sb")
            for m in range(KC):
                nc.scalar.activation(q_sb[:, m, :], qps[:, m, :], AF.Identity,
                                     bias=dwb[:, m:m + 1])
            o_d = smalls.tile([128, KC, BL], F16, tag="o_d")
            for s in range(BL):
                sl = slice(s * S, (s + 1) * S)
                # K (feature-major) and V' (token-major) over morph_enc
                k_sb = a4.tile([128, KC, S], F16, tag="at", bufs=3)
                for m in range(KC):
                    ps = b1.tile([128, S], F32, tag="b1")
                    for k in range(KC):
                        nc.tensor.matmul(
                            ps[:], dw[:, k, D + m * 128:D + (m + 1) * 128],
                            me[:, k, sl], start=(k == 0), stop=(k == KC - 1))
                    nc.scalar.activation(k_sb[:, m, :], ps[:], AF.Identity,
                                         bias=dwb[:, 4 + m:5 + m])
                vloc = vp.tile([128, KC, 8, 65], F16, tag="vloc")
                for t in range(KC):
                    nc.vector.tensor_copy(vloc[:, t, :, 64], ones8[:])
                for t in range(KC):
                    ps = b1.tile([128, S], F32, tag="b1")
                    for k in range(KC):
                        nc.tensor.matmul(
                            ps[:], me[:, k, s * S + t * 128:s * S + (t + 1) * 128],
                            dw[:, k, 2 * D:3 * D],
                            start=(k == 0), stop=(k == KC - 1))
                    nc.vector.tensor_tensor(
                        vloc[:, t, :, 0:64],
                        ps[:].rearrange("p (h d) -> p h d", h=H),
                        vb_bc[:].rearrange("p (h d) -> p h d", h=H), ALU.add)
                scp = b1.tile([128, KC, H], F32, tag="b1")
                for hh in range(H):
                    rows = slice(64 * (hh % 2), 64 * (hh % 2) + 64)
                    for c in range(KC):
                        nc.tensor.matmul(
                            scp[:, c, hh:hh + 1],
                            k_sb[rows, hh // 2, c * 128:(c + 1) * 128],
                            q_sb[rows, hh // 2, s:s + 1],
                            start=True, stop=True)
                at = smalls.tile([128, KC, H], F16, tag="at_d")
                nc.scalar.activation(at[:], scp[:], AF.Exp,
                                     scale=float(1.0 / np.sqrt(DH)))
                ov = b2.tile([65, H], F32, tag="b2")
                for hh in range(H):
                    for c in range(KC):
                        nc.tensor.matmul(ov[:, hh:hh + 1], vloc[:, c, hh, :],
                                         at[:, c, hh:hh + 1],
                                         start=(c == 0), stop=(c == KC - 1))
                # 1/denom = exp(-ln denom)
                lnd = scr.tile([1, H], F16, tag="lnd")
                nc.scalar.activation(lnd[:], ov[64:65, :], AF.Ln)
                rb = b2.tile([64, H], F32, tag="b2")
                nc.tensor.matmul(rb[:], ones_row[:, 0:64], lnd[:],
                                 start=True, stop=True)
                inv = invp.tile([64, H], F16, tag="inv_d")
                nc.scalar.activation(inv[:], rb[:], AF.Exp, scale=-1.0)
                for hh in range(H):
                    rows = slice(64 * (hh % 2), 64 * (hh % 2) + 64)
                    nc.vector.tensor_tensor(o_d[rows, hh // 2, s:s + 1],
                                            ov[0:64, hh:hh + 1],
                                            inv[:, hh:hh + 1], ALU.mult)
            # out-proj + residual into p
            ops = b1.tile([128, KC, BL], F32, tag="b1")
            for m in range(KC):
                for k in range(KC):
                    nc.tensor.matmul(ops[:, m, :],
                                     do[:, k, m * 128:(m + 1) * 128],
                                     o_d[:, k, :], start=(k == 0),
                                     stop=(k == KC - 1))
            for m in range(KC):
                nc.vector.scalar_tensor_tensor(
                    p[:, m, :], ops[:, m, :], dob[:, m:m + 1], p[:, m, :],
                    ALU.add, ALU.add)
            # FFN on p (m2 streams in chunked once dw releases its slot)
            m2 = wpool.tile([128, MC, D], F16, tag="bigw")
            for kf in range(MC):
                nc.sync.dma_start(m2[:, kf, :], dram["dm2T"][li][:, kf, :])
            h2d = smalls.tile([128, KC, BL], F16, tag="q_ln")
            ln_small(p, BL, h2d)
            mh = smalls.tile([128, MC, BL], F16, tag="mh")
            for mm_ in range(MC):
                ps = b1.tile([128, BL], F32, tag="b1")
                for k in range(KC):
                    nc.tensor.matmul(ps[:], m1[:, k, mm_ * 128:(mm_ + 1) * 128],
                                     h2d[:, k, :], start=(k == 0),
                                     stop=(k == KC - 1))
                nc.scalar.activation(mh[:, mm_, :], ps[:], AF.Relu,
                                     bias=m1b[:, mm_:mm_ + 1])
            m2ps = b1.tile([128, KC, BL], F32, tag="b1")
            for m in range(KC):
                for kf in range(MC):
                    nc.tensor.matmul(m2ps[:, m, :],
                                     m2[:, kf, m * 128:(m + 1) * 128],
                                     mh[:, kf, :], start=(kf == 0),
                                     stop=(kf == MC - 1))
            for m in range(KC):
                nc.vector.scalar_tensor_tensor(
                    p[:, m, :], m2ps[:, m, :], m2b[:, m:m + 1], p[:, m, :],
                    ALU.add, ALU.add)

        # ---------------- head ----------------
        hw = smalls.tile([128, KC], F16, tag="hw")
        hb = smalls.tile([1, 1], F32, tag="hb")
        nc.sync.dma_start(hw[:], dram["hwT"][:])
        nc.sync.dma_start(hb[:], dram["hb"][:])
        hg = smalls.tile([128, KC, BL], F16, tag="q_ln")
        ln_small(p, BL, hg)
        hps = b2.tile([1, BL], F32, tag="b2")
        for k in range(KC):
            nc.tensor.matmul(hps[:], hw[:, k:k + 1], hg[:, k, :],
                             start=(k == 0), stop=(k == KC - 1))
        y_sb = smalls.tile([1, BL], F32, tag="y_sb")
        nc.scalar.activation(y_sb[:], hps[:], AF.Sigmoid, bias=hb[:])
        nc.sync.dma_start(y_dram[:], y_sb[:])


# ----------------------------------------------------------------------------
# entry point
# ----------------------------------------------------------------------------

_NC_CACHE = {}


def kernel(**inputs):
    return _run(inputs, LE, LD)


def _run(inputs, le, ld, trace=False):
    w = prep_weights(inputs, le, ld)
    morph = np.asarray(inputs["morph"], np.float32)
    pose = np.asarray(inputs["pose"], np.float32)
    in_maps = []
    for c in range(NCORES):
        im = dict(w)
        mo = morph[c * BL:(c + 1) * BL]                 # [BL, S, 3]
        im["morphT"] = np.ascontiguousarray(
            mo.transpose(2, 0, 1).reshape(3, T)).astype(np.float16)
        im["poseT"] = np.ascontiguousarray(
            pose[c * BL:(c + 1) * BL].T).astype(np.float16)
        in_maps.append(im)

    if ("nc", le, ld) not in _NC_CACHE:
        _NC_CACHE[("nc", le, ld)] = build(le, ld)
    nc = _NC_CACHE[("nc", le, ld)]
    res = run_bass_kernel_spmd(nc, in_maps, core_ids=list(range(NCORES)),
                               trace=trace)
    out = np.zeros((B, 1), np.float32)
    for c in range(NCORES):
        out[c * BL:(c + 1) * BL, 0] = res.results[c]["y"][0]
    if trace:
        return out, res
    return out


# revision 37
# speedup vs baseline: 1.7853x; 1.0632x over previous
"""Trainium2 Bass kernel for nn_ReachabilityClassifierTransformer.

Data-parallel over batch: 16 samples / 8 cores = 2 samples per core.
Each core runs the full network (6-layer encoder + 4-layer decoder + head)
on its 2 samples. No collectives.

v2 (fp16 pipeline):
  - All matmul operands fp16 (stationary weights get Fast-Weight-Load; DVE
    elementwise ops run in 2x mode; DMA traffic halved). PSUM stays fp32.
  - V path folded on host: V = h @ (Wv2 Wv1)^T + bv  (no rope between the
    two V projections, so the double-projection quirk collapses).
  - Stage-1 q/k biases folded through rope into per-position bias tensors
    C_q/C_k = W2 @ rope(b1) + b2, added at the stage-2 PSUM copy (rope is
    linear, rotation depends only on position).
  - No Sqrt / no DVE reciprocal anywhere: LN rsqrt = exp(-0.5 ln(v+eps)),
    softmax 1/denom = exp(-ln denom) broadcast via PE.  ln/exp/copy/relu/
    square all live in one activation table set -> no table switches.
  - Per-layer phase order interleaves the two samples so rope (DVE) and
    softmax exp (Act) hide under the other sample's matmuls.

Device layout conventions (per core):
  - Activations FEATURE-MAJOR in SBUF: tile [128, KC, T] holds X.T.
  - Weights pre-transposed on host to [in_feat, out_feat], laid out
    [128, KC_in, O] (partition = in-feature % 128).
  - matmul(out_psum[M,N], lhsT=[K,M], rhs=[K,N]) computes lhsT.T @ rhs.
  - Encoder stage-1 q,k output features are de-interleaved (even feats then
    odd feats) via host-side column permutation, so RoPE becomes contiguous
    block ops; the roped result is in natural (concatenated) order.
  - Softmax: scores computed transposed (S.T = K_h @ Q_h.T per chunk),
    exp'd without max subtraction (|scores/8| < 1 for this model), and the
    denominator comes free from a ones-column appended to V.
"""
import functools

import numpy as np

import concourse.bass as bass
import concourse.mybir as mybir
import concourse.tile as tile
from concourse import bacc
from concourse.bass_utils import run_bass_kernel_spmd


def _patch_act_tables():
    """Constrain exp/ln to the one table set that contains both.

    The act-table-load pass maps each activation function to a set
    independently (exp -> exp_and_others, ln -> natural_log), so a kernel
    that interleaves exp and ln reloads tables on every transition
    (~1.3us each).  natural_log_exp_and_others contains exp AND ln (plus
    copy/identity/relu/square), so restricting exp/ln to that set makes
    every load resolve there; set ids/order are preserved so the emitted
    act_func_set_id still indexes the real act_info.json.
    """
    import concourse.hw_specs as hw_specs
    if getattr(hw_specs, "_ant_act_tables_patched", False):
        return
    orig = hw_specs.get_activation_tables

    @functools.cache
    def patched(module_arch):
        t = orig(module_arch)
        keep = "natural_log_exp_and_others"
        if keep not in t:
            return t
        drop = {mybir.ActivationFunctionType.Exp, mybir.ActivationFunctionType.Ln}
        return {name: (fns if name == keep else fns - drop)
                for name, fns in t.items()}

    hw_specs._ant_act_tables_patched = True
    hw_specs.get_activation_tables = patched
    import sys
    for modname in ("concourse.bacc", "concourse.bass_interp"):
        mod = sys.modules.get(modname)
        if mod is not None and hasattr(mod, "get_activation_tables"):
            mod.get_activation_tables = patched


_patch_act_tables()

AF = mybir.ActivationFunctionType
ALU = mybir.AluOpType
F32 = mybir.dt.float32
F16 = mybir.dt.float16
F8 = mybir.dt.float8e4
F8NP = mybir.dt.np(F8)
DR = mybir.MatmulPerfMode.DoubleRow
F8MAX = 240.0          # TRN fp8e4 saturation (not OCP's 448)
WS = 64.0              # fp8 weight scale
HS = 16.0              # fp8 activation scale (h, h2, roped q/k)
OS = 32.0              # fp8 o_t / FFN-hidden scale
WHS = WS * HS          # stage-1/2/V psum descale
WOS = WS * OS          # out-proj / FFN-l2 psum descale

B, S, D, FF, H, LE, LD, M = 16, 512, 512, 2048, 8, 6, 4, 2048
ROPE_BASE = 10000.0
LN_EPS = 1e-5
NCORES = 8
BL = B // NCORES          # 2 samples per core
T = BL * S                # 1024 tokens per core
KC = D // 128             # 4 feature chunks
FC = FF // 128            # 16
MC = M // 128             # 16
DH = D // H               # 64


# ----------------------------------------------------------------------------
# host-side helpers
# ----------------------------------------------------------------------------

def _chunked(wT):
    """[Din, O] -> [128, Din//128, O] contiguous fp16."""
    Din, O = wT.shape
    return np.ascontiguousarray(
        wT.reshape(Din // 128, 128, O).transpose(1, 0, 2)).astype(np.float16)


def _chunked8(wT, scale=WS):
    """[Din, O] -> [128, Din//128, O] contiguous fp8e4, pre-scaled."""
    Din, O = wT.shape
    a = np.clip(wT * scale, -F8MAX, F8MAX)
    return np.ascontiguousarray(
        a.reshape(Din // 128, 128, O).transpose(1, 0, 2)).astype(F8NP)


def _bias_cols(b):
    """[O] -> [128, O//128]  (column per 128-chunk), fp32."""
    O = b.shape[0]
    return np.ascontiguousarray(b.reshape(O // 128, 128).T).astype(np.float32)


_DEINT = np.concatenate([np.arange(0, D, 2), np.arange(1, D, 2)])  # de-interleave


def prep_weights(inp, le=LE, ld=LD):
    """Host-side weight prep -> dict of arrays shared by all cores."""
    out = {}
    g = {k: np.asarray(v, np.float64) for k, v in inp.items()}

    out["mpwT"] = np.ascontiguousarray(g["morph_proj_w"].T).astype(np.float16)
    out["mpb"] = _bias_cols(g["morph_proj_b"])                     # [128, 4]
    out["ppwT"] = np.ascontiguousarray(g["pose_proj_w"].T).astype(np.float16)
    out["ppb"] = _bias_cols(g["pose_proj_b"])

    # rope grids, de-interleaved frequency order: [128, 2, 512] fp16.
    # Pre-scaled by HS so rope's final DVE op writes HS*rope(q) in fp8.
    freq = 1.0 / ROPE_BASE ** (np.arange(0, D, 2, dtype=np.float64) / D)
    ang = np.outer(np.arange(S, dtype=np.float64), freq)           # [512, 256]
    cosT = np.cos(ang).T                                           # [256, S]
    sinT = np.sin(ang).T
    out["gridc"] = _chunked(HS * cosT.reshape(256, S))
    out["grids"] = _chunked(HS * sinT.reshape(256, S))

    e_w1, e_w2, e_wv, e_cqk, e_vb = [], [], [], [], []
    e_ow, e_owb, e_l1, e_l1b, e_l2, e_l2b = [], [], [], [], [], []
    for i in range(le):
        w1 = g["enc_in_w"][i] * g["enc_n1_g"][i][None, :]          # fold n1 g
        b1 = g["enc_in_b"][i] + g["enc_in_w"][i] @ g["enc_n1_b"][i]
        # stage-1 q,k only, de-interleaved output columns
        perm = np.concatenate([_DEINT, D + _DEINT])
        e_w1.append(_chunked8(np.ascontiguousarray(w1[perm].T)))   # [128,4,1024]
        # stage-2 q,k (natural order, raw weights - the faithful quirk)
        w2 = g["enc_in_w"][i][: 2 * D]                             # Wq;Wk
        e_w2.append(_chunked8(np.ascontiguousarray(w2.T)))         # [128,4,1024]
        # stage-2 bias tensors: C = W2 @ rope(b1) + b2   [512, S] each
        bq = b1[:D][_DEINT]                                        # [even; odd]
        bk = b1[D:2 * D][_DEINT]
        rb_q = np.concatenate([bq[:256, None] * cosT - bq[256:, None] * sinT,
                               bq[:256, None] * sinT + bq[256:, None] * cosT])
        rb_k = np.concatenate([bk[:256, None] * cosT - bk[256:, None] * sinT,
                               bk[:256, None] * sinT + bk[256:, None] * cosT])
        Cq = g["enc_in_w"][i][:D] @ rb_q + g["enc_in_b"][i][:D][:, None]
        Ck = g["enc_in_w"][i][D:2 * D] @ rb_k \
            + g["enc_in_b"][i][D:2 * D][:, None]
        C = np.concatenate([Cq, Ck], axis=0)                       # [1024, S]
        e_cqk.append(_chunked(C))                                  # [128,8,S]
        # V folded: V = h @ (Wv2 Wv1_f).T + (Wv2 bv1_f + bv2)
        Wv1f = w1[2 * D:]
        bv1f = b1[2 * D:]
        Wv2 = g["enc_in_w"][i][2 * D:]
        bv2 = g["enc_in_b"][i][2 * D:]
        e_wv.append(_chunked8(np.ascontiguousarray((Wv2 @ Wv1f).T)))
        e_vb.append((Wv2 @ bv1f + bv2)[None, :].astype(np.float16))  # [1,512]
        e_ow.append(_chunked8(np.ascontiguousarray(g["enc_out_w"][i].T)))
        e_owb.append((WOS * g["enc_out_b"][i])[None, :].astype(np.float16))
        l1 = g["enc_l1_w"][i] * g["enc_n2_g"][i][None, :]
        l1b = g["enc_l1_b"][i] + g["enc_l1_w"][i] @ g["enc_n2_b"][i]
        e_l1.append(_chunked8(np.ascontiguousarray(l1.T)))         # [128,4,2048]
        e_l1b.append(_bias_cols(OS * l1b))                         # [128,16]
        e_l2.append(_chunked8(np.ascontiguousarray(g["enc_l2_w"][i].T)))
        e_l2b.append((WOS * g["enc_l2_b"][i])[None, :].astype(np.float16))
    out["ew1T"] = np.stack(e_w1) if le else np.zeros((0, 128, KC, 2 * D), F8NP)
    out["ew2T"] = np.stack(e_w2) if le else np.zeros((0, 128, KC, 2 * D), F8NP)
    out["ewvT"] = np.stack(e_wv) if le else np.zeros((0, 128, KC, D), F8NP)
    out["ecqk"] = np.stack(e_cqk) if le else np.zeros((0, 128, 8, S), np.float16)
    out["evb"] = np.stack(e_vb) if le else np.zeros((0, 1, D), np.float16)
    out["eowT"] = np.stack(e_ow) if le else np.zeros((0, 128, KC, D), F8NP)
    out["eowb"] = np.stack(e_owb) if le else np.zeros((0, 1, D), np.float16)
    out["el1T"] = np.stack(e_l1) if le else np.zeros((0, 128, KC, FF), F8NP)
    out["el1b"] = np.stack(e_l1b) if le else np.zeros((0, 128, 16), np.float32)
    out["el2T"] = np.stack(e_l2) if le else np.zeros((0, 128, FC, D), F8NP)
    out["el2b"] = np.stack(e_l2b) if le else np.zeros((0, 1, D), np.float16)

    d_in, d_inb, d_vb, d_ow, d_owb = [], [], [], [], []
    d_m1, d_m1b, d_m2, d_m2b = [], [], [], []
    for i in range(ld):
        w = g["dec_in_w"][i].copy()
        b = g["dec_in_b"][i].copy()
        w[:D] = w[:D] * g["dec_n1_g"][i][None, :]                  # Wq <- dec_n1
        b[:D] = b[:D] + g["dec_in_w"][i][:D] @ g["dec_n1_b"][i]
        w[D:] = w[D:] * g["enc_final_g"][None, :]                  # Wk,Wv <- enc_final
        b[D:] = b[D:] + g["dec_in_w"][i][D:] @ g["enc_final_b"]
        d_in.append(_chunked(np.ascontiguousarray(w.T)))           # [128,4,1536]
        d_inb.append(_bias_cols(b))
        d_vb.append(b[2 * D:][None, :].astype(np.float16))         # [1,512]
        d_ow.append(_chunked(np.ascontiguousarray(g["dec_out_w"][i].T)))
        d_owb.append(_bias_cols(g["dec_out_b"][i]))
        m1 = g["dec_m1_w"][i] * g["dec_n2_g"][i][None, :]
        m1b = g["dec_m1_b"][i] + g["dec_m1_w"][i] @ g["dec_n2_b"][i]
        d_m1.append(_chunked(np.ascontiguousarray(m1.T)))          # [128,4,2048]
        d_m1b.append(_bias_cols(m1b))
        d_m2.append(_chunked(np.ascontiguousarray(g["dec_m2_w"][i].T)))
        d_m2b.append(_bias_cols(g["dec_m2_b"][i]))
    out["dinT"] = np.stack(d_in) if ld else np.zeros((0, 128, KC, 3 * D), np.float16)
    out["dinb"] = np.stack(d_inb) if ld else np.zeros((0, 128, 12), np.float32)
    out["dvb"] = np.stack(d_vb) if ld else np.zeros((0, 1, D), np.float16)
    out["dowT"] = np.stack(d_ow) if ld else np.zeros((0, 128, KC, D), np.float16)
    out["dowb"] = np.stack(d_owb) if ld else np.zeros((0, 128, 4), np.float32)
    out["dm1T"] = np.stack(d_m1) if ld else np.zeros((0, 128, KC, M), np.float16)
    out["dm1b"] = np.stack(d_m1b) if ld else np.zeros((0, 128, 16), np.float32)
    out["dm2T"] = np.stack(d_m2) if ld else np.zeros((0, 128, MC, D), np.float16)
    out["dm2b"] = np.stack(d_m2b) if ld else np.zeros((0, 128, 4), np.float32)

    hw = (g["head_w"] * g["head_g"][None, :])[0]                   # [512]
    out["hwT"] = _bias_cols(hw).astype(np.float16)                 # [128, 4]
    out["hb"] = (g["head_bias"] + g["head_w"] @ g["head_b"]).reshape(1, 1).astype(np.float32)
    return out


# ----------------------------------------------------------------------------
# device program
# ----------------------------------------------------------------------------

def build(le=LE, ld=LD):
    nc = bacc.Bacc(None, target_bir_lowering=False)

    dram = {}

    def din(name, shape, dt=F16):
        dram[name] = nc.dram_tensor(name, list(shape), dt, kind="ExternalInput")
        return dram[name]

    # shared weights
    din("mpwT", [3, 512]); din("mpb", [128, 4], F32)
    din("ppwT", [9, 512]); din("ppb", [128, 4], F32)
    din("gridc", [128, 2, S]); din("grids", [128, 2, S])
    din("ew1T", [le, 128, KC, 2 * D], F8)
    din("ew2T", [le, 128, KC, 2 * D], F8)
    din("ewvT", [le, 128, KC, D], F8)
    din("ecqk", [le, 128, 8, S])
    din("evb", [le, 1, D])
    din("eowT", [le, 128, KC, D], F8); din("eowb", [le, 1, D])
    din("el1T", [le, 128, KC, FF], F8); din("el1b", [le, 128, 16], F32)
    din("el2T", [le, 128, FC, D], F8); din("el2b", [le, 1, D])
    din("dinT", [ld, 128, KC, 3 * D]); din("dinb", [ld, 128, 12], F32)
    din("dvb", [ld, 1, D])
    din("dowT", [ld, 128, KC, D]); din("dowb", [ld, 128, 4], F32)
    din("dm1T", [ld, 128, KC, M]); din("dm1b", [ld, 128, 16], F32)
    din("dm2T", [ld, 128, MC, D]); din("dm2b", [ld, 128, 4], F32)
    din("hwT", [128, KC]); din("hb", [1, 1], F32)
    # per-core inputs
    din("morphT", [3, T])
    din("poseT", [9, BL])
    y = nc.dram_tensor("y", [1, BL], F32, kind="ExternalOutput")

    with tile.TileContext(nc) as tc:
        _build_body(nc, tc, dram, y, le, ld)
    nc.compile()
    return nc


def _build_body(nc, tc, dram, y_dram, le, ld):
    import contextlib
    ctx = contextlib.ExitStack()
    with ctx:
        ctx.enter_context(nc.allow_low_precision(
            reason="fp16 matmul operands / activations are intentional"))
        persist = ctx.enter_context(tc.tile_pool(name="persist", bufs=1))
        wpool = ctx.enter_context(tc.tile_pool(name="wpool", bufs=2))
        w2pool = ctx.enter_context(tc.tile_pool(name="w2pool", bufs=1))
        owpool = ctx.enter_context(tc.tile_pool(name="owpool", bufs=1))
        vwpool = ctx.enter_context(tc.tile_pool(name="vwpool", bufs=1))
        cqpool = ctx.enter_context(tc.tile_pool(name="cqpool", bufs=1))
        bpool = ctx.enter_context(tc.tile_pool(name="bpool", bufs=2))
        a4 = ctx.enter_context(tc.tile_pool(name="a4", bufs=4))
        hp = ctx.enter_context(tc.tile_pool(name="hp", bufs=2))
        a8 = ctx.enter_context(tc.tile_pool(name="a8", bufs=3))
        qk2p = ctx.enter_context(tc.tile_pool(name="qk2p", bufs=2))
        vp = ctx.enter_context(tc.tile_pool(name="vp", bufs=2))
        rtp = ctx.enter_context(tc.tile_pool(name="rtp", bufs=3))
        vbp = ctx.enter_context(tc.tile_pool(name="vbp", bufs=1))
        scr = ctx.enter_context(tc.tile_pool(name="scr", bufs=3))
        invp = ctx.enter_context(tc.tile_pool(name="invp", bufs=2))
        rcp = ctx.enter_context(tc.tile_pool(name="rcp", bufs=2))
        smalls = ctx.enter_context(tc.tile_pool(name="smalls", bufs=2))
        b1 = ctx.enter_context(tc.tile_pool(name="b1", bufs=4, space="PSUM"))
        b2 = ctx.enter_context(tc.tile_pool(name="b2", bufs=4, space="PSUM"))

        # ---------------- persistent tiles ----------------
        x = persist.tile([128, KC, T], F16)           # residual stream (X.T)
        gridc = persist.tile([128, 2, S], F16)
        grids = persist.tile([128, 2, S], F16)
        ones128 = persist.tile([128, 1], F16)
        ones_row = persist.tile([1, 128], F16)
        ones_rowS = persist.tile([1, S], F16)         # bias-inject moving row
        hs_row = persist.tile([1, 128], F16)          # HS-scaled broadcast row
        ones8 = persist.tile([128, 8], F8)
        eps_t = persist.tile([1, 1], F32)
        ln_os = persist.tile([64, 1], F32)            # ln(OS) bias for 1/denom
        p = persist.tile([128, KC, BL], F16)          # decoder latent p.T
        nc.sync.dma_start(gridc[:], dram["gridc"][:])
        nc.sync.dma_start(grids[:], dram["grids"][:])
        stage_f16 = rtp.tile([128, 128], F16, tag="rt")
        nc.vector.memset(stage_f16[:], 1.0)
        nc.vector.tensor_copy(ones128[:], stage_f16[:, 0:1])
        nc.vector.tensor_copy(ones_row[:], stage_f16[0:1, :])
        nc.vector.tensor_copy(ones8[:], stage_f16[:, 0:8])
        nc.vector.memset(hs_row[:], HS)
        nc.vector.memset(ones_rowS[:], 1.0)
        nc.vector.memset(eps_t[:], LN_EPS)
        nc.vector.memset(ln_os[:], float(np.log(OS)))

        def ln_small(x_tile, n_tok, h_out):
            """h_out = LayerNorm_features(x_tile) for tiny n_tok (decoder)."""
            sq = smalls.tile([128, KC, n_tok], F16, tag="sq_d")
            for k in range(KC):
                nc.vector.tensor_tensor(sq[:, k, :], x_tile[:, k, :],
                                        x_tile[:, k, :], ALU.mult)
            sum_ps = b2.tile([1, n_tok], F32, tag="b2")
            sq_ps = b2.tile([1, n_tok], F32, tag="b2")
            for k in range(KC):
                nc.tensor.matmul(sum_ps[:], ones128[:], x_tile[:, k, :],
                                 start=(k == 0), stop=(k == KC - 1))
            for k in range(KC):
                nc.tensor.matmul(sq_ps[:], ones128[:], sq[:, k, :],
                                 start=(k == 0), stop=(k == KC - 1))
            ms = scr.tile([1, n_tok], F32, tag="scr")
            t2 = scr.tile([1, n_tok], F32, tag="scr")
            rc = scr.tile([1, 2, n_tok], F16, tag="scr_rc")
            nc.scalar.activation(ms[:], sum_ps[:], AF.Copy, scale=1.0 / D)
            nc.vector.tensor_tensor(t2[:], ms[:], ms[:], ALU.mult)      # m^2
            nc.vector.scalar_tensor_tensor(
                t2[:], sq_ps[:], 1.0 / D, t2[:], ALU.mult, ALU.subtract)
            # r = exp(-0.5 ln(var + eps))
            nc.scalar.activation(t2[:], t2[:], AF.Ln, bias=eps_t[:])
            nc.scalar.activation(rc[:, 0, :], t2[:], AF.Exp, scale=-0.5)
            nc.vector.scalar_tensor_tensor(
                rc[:, 1, :], ms[:], -1.0, rc[:, 0, :], ALU.mult, ALU.mult)
            r_ps = b1.tile([128, n_tok], F32, tag="b1")
            c_ps = b1.tile([128, n_tok], F32, tag="b1")
            nc.tensor.matmul(r_ps[:], ones_row[:], rc[:, 0, :],
                             start=True, stop=True)
            nc.tensor.matmul(c_ps[:], ones_row[:], rc[:, 1, :],
                             start=True, stop=True)
            for k in range(KC):
                nc.vector.tensor_tensor(h_out[:, k, :], x_tile[:, k, :],
                                        r_ps[:], ALU.mult)
                nc.vector.tensor_tensor(h_out[:, k, :], h_out[:, k, :],
                                        c_ps[:], ALU.add)

        def ln_stats(x_tile, s):
            """Per-sample LN stats -> rc [1, 2, S] fp16 (r, c)."""
            sl = slice(s * S, (s + 1) * S)
            sq = a4.tile([128, KC, S], F16, tag="sq", bufs=2)
            nc.vector.tensor_tensor(sq[:], x_tile[:, :, sl], x_tile[:, :, sl],
                                    ALU.mult)
            sum_ps = b2.tile([1, S], F32, tag="b2")
            sq_ps = b2.tile([1, S], F32, tag="b2")
            for k in range(KC):
                nc.tensor.matmul(sum_ps[:], ones128[:], x_tile[:, k, sl],
                                 start=(k == 0), stop=(k == KC - 1))
            for k in range(KC):
                nc.tensor.matmul(sq_ps[:], ones128[:], sq[:, k, :],
                                 start=(k == 0), stop=(k == KC - 1))
            ms = scr.tile([1, S], F32, tag="scr")
            t2 = scr.tile([1, S], F32, tag="scr")
            rc = rcp.tile([1, 2, S], F16, tag="rc", bufs=4)
            nc.scalar.activation(ms[:], sum_ps[:], AF.Copy, scale=1.0 / D)
            nc.vector.tensor_tensor(t2[:], ms[:], ms[:], ALU.mult)
            nc.vector.scalar_tensor_tensor(
                t2[:], sq_ps[:], 1.0 / D, t2[:], ALU.mult, ALU.subtract)
            nc.scalar.activation(t2[:], t2[:], AF.Ln, bias=eps_t[:])
            nc.scalar.activation(rc[:, 0, :], t2[:], AF.Exp, scale=-0.5)
            nc.vector.scalar_tensor_tensor(
                rc[:, 1, :], ms[:], -1.0, rc[:, 0, :], ALU.mult, ALU.mult)
            return rc

        def ln_apply(rc, x_tile, s, h_out, scaled=False):
            """h_out = (x[:, :, s] * r + c) * (HS if scaled else 1).

            r,c broadcast via PE; the HS factor rides on the broadcast row.
            """
            sl = slice(s * S, (s + 1) * S)
            row = hs_row if scaled else ones_row
            r_ps = b1.tile([128, S], F32, tag="b1")
            c_ps = b1.tile([128, S], F32, tag="b1")
            nc.tensor.matmul(r_ps[:], row[:], rc[:, 0, :],
                             start=True, stop=True)
            nc.tensor.matmul(c_ps[:], row[:], rc[:, 1, :],
                             start=True, stop=True)
            rcb = rcp.tile([128, 2, S], F16, tag="rcb")
            nc.scalar.activation(rcb[:, 0, :], r_ps[:], AF.Copy)
            nc.scalar.activation(rcb[:, 1, :], c_ps[:], AF.Copy)
            for k in range(KC):
                tmp = rtp.tile([128, S], F16, tag="rt")
                nc.vector.tensor_tensor(tmp[:], x_tile[:, k, sl],
                                        rcb[:, 0, :], ALU.mult)
                nc.vector.tensor_tensor(h_out[:, k, :], tmp[:],
                                        rcb[:, 1, :], ALU.add)

        # ---------------- morph projection -> x ----------------
        morpht = rtp.tile([3, T], F16, tag="morph", bufs=1)
        nc.sync.dma_start(morpht[:], dram["morphT"][:])
        mpw = rtp.tile([3, 512], F16, tag="rt")
        mpb = rtp.tile([128, 4], F32, tag="rtb")
        nc.sync.dma_start(mpw[:], dram["mpwT"][:])
        nc.sync.dma_start(mpb[:], dram["mpb"][:])
        rc1 = {}
        for s in range(BL):
            for m in range(KC):
                ps = b1.tile([128, S], F32, tag="b1")
                nc.tensor.matmul(ps[:], mpw[:, m * 128:(m + 1) * 128],
                                 morpht[:, s * S:(s + 1) * S], start=True, stop=True)
                nc.scalar.activation(x[:, m, s * S:(s + 1) * S], ps[:], AF.Relu,
                                     bias=mpb[:, m:m + 1])
            rc1[s] = ln_stats(x, s)

        # ---------------- pose projection -> p ----------------
        poset = rtp.tile([9, BL], F16, tag="rt")
        ppw = rtp.tile([9, 512], F16, tag="rt")
        ppb = rtp.tile([128, 4], F32, tag="rtb")
        nc.sync.dma_start(poset[:], dram["poseT"][:])
        nc.sync.dma_start(ppw[:], dram["ppwT"][:])
        nc.sync.dma_start(ppb[:], dram["ppb"][:])
        pps = b1.tile([128, KC, BL], F32, tag="b1")
        for m in range(KC):
            nc.tensor.matmul(pps[:, m, :], ppw[:, m * 128:(m + 1) * 128],
                             poset[:], start=True, stop=True)
        for m in range(KC):
            nc.scalar.activation(p[:, m, :], pps[:, m, :], AF.Relu,
                                 bias=ppb[:, m:m + 1])

        # ---------------- encoder layers ----------------
        for li in range(le):
            w1 = wpool.tile([128, KC, 2 * D], F8, tag="bigw")
            nc.sync.dma_start(w1[:], dram["ew1T"][li])
            w2 = w2pool.tile([128, KC, 2 * D], F8, tag="w2")
            nc.sync.dma_start(w2[:], dram["ew2T"][li])
            wv = vwpool.tile([128, KC, D], F8, tag="wv")
            nc.sync.dma_start(wv[:], dram["ewvT"][li])
            cq = cqpool.tile([128, 8, S], F16, tag="cq")
            nc.sync.dma_start(cq[:], dram["ecqk"][li])
            vbrow = bpool.tile([1, D], F16, tag="vbrow")
            nc.sync.dma_start(vbrow[:], dram["evb"][li])
            ow = owpool.tile([128, KC, D], F8, tag="ow")
            nc.sync.dma_start(ow[:], dram["eowT"][li])
            owb = bpool.tile([1, D], F16, tag="owbr")
            nc.sync.dma_start(owb[:], dram["eowb"][li])
            l1 = wpool.tile([128, KC, FF], F8, tag="bigw")
            nc.sync.dma_start(l1[:], dram["el1T"][li])
            l1b = bpool.tile([128, 16], F32, tag="l1b")
            nc.sync.dma_start(l1b[:], dram["el1b"][li])
            l2b = bpool.tile([1, D], F16, tag="l2br")
            nc.sync.dma_start(l2b[:], dram["el2b"][li])

            # v-bias broadcast [128, 512], once per layer
            vb_ps = b1.tile([128, D], F32, tag="b1")
            nc.tensor.matmul(vb_ps[:], ones_row[:], vbrow[:], start=True, stop=True)
            vb_bc = vbp.tile([128, D], F16, tag="vb_bc")
            nc.scalar.activation(vb_bc[:], vb_ps[:], AF.Copy)

            qkrs, vlocs = {}, {}
            # ---- phase A1 per sample: LN1-apply, stage1 q,k; V; rope ----
            # (rc1 stats were emitted at the end of the previous phase so the
            #  scalar chain overlapped that phase's matmuls)
            for s in range(BL):
                h = hp.tile([128, KC, S], F8, tag="h")
                ln_apply(rc1[s], x, s, h, scaled=True)   # h = HS * LN(x)
                qkv1 = a8.tile([128, 8, S], F16, tag="a8")
                for m in range(8):
                    ps = b1.tile([128, S], F32, tag="b1")
                    for c in range(2):
                        nc.tensor.matmul(ps[:],
                                         w1[:, 2 * c:2 * c + 2, m * 128:(m + 1) * 128],
                                         h[:, 2 * c:2 * c + 2, :],
                                         start=(c == 0), stop=(c == 1),
                                         perf_mode=DR)
                    nc.scalar.activation(qkv1[:, m, :], ps[:], AF.Copy,
                                         scale=1.0 / WHS)
                # V token-major (+ones col), from h directly (folded weights)
                vloc = vp.tile([128, KC, 8, 72], F8, tag="vloc")
                for t in range(KC):
                    nc.vector.tensor_copy(vloc[:, t, :, 64], ones8[:])
                for t in range(KC):
                    ps = b1.tile([128, S], F32, tag="b1")
                    for c in range(2):
                        nc.tensor.matmul(
                            ps[:], h[:, 2 * c:2 * c + 2, t * 128:(t + 1) * 128],
                            wv[:, 2 * c:2 * c + 2, :],
                            start=(c == 0), stop=(c == 1), perf_mode=DR)
                    nc.vector.scalar_tensor_tensor(
                        vloc[:, t, :, 0:64],
                        ps[:].rearrange("p (h d) -> p h d", h=H), 1.0 / WHS,
                        vb_bc[:].rearrange("p (h d) -> p h d", h=H),
                        ALU.mult, ALU.add)
                vlocs[s] = vloc
                # rope: qkv1 (de-interleaved) -> qkr = HS*rope(qk) in fp8
                # (grids are pre-scaled by HS on host)
                qkr = a8.tile([128, 8, S], F8, tag="a8")
                for half in (0, 4):
                    for c in range(2):
                        e = qkv1[:, half + c, :]
                        o = qkv1[:, half + 2 + c, :]
                        ta = rtp.tile([128, S], F16, tag="rt")
                        tb = rtp.tile([128, S], F16, tag="rt")
                        nc.vector.tensor_tensor(ta[:], e, gridc[:, c, :], ALU.mult)
                        nc.vector.tensor_tensor(tb[:], o, grids[:, c, :], ALU.mult)
                        nc.vector.tensor_tensor(qkr[:, half + c, :], ta[:], tb[:],
                                                ALU.subtract)
                        tc = rtp.tile([128, S], F16, tag="rt")
                        td = rtp.tile([128, S], F16, tag="rt")
                        nc.vector.tensor_tensor(tc[:], e, grids[:, c, :], ALU.mult)
                        nc.vector.tensor_tensor(td[:], o, gridc[:, c, :], ALU.mult)
                        nc.vector.tensor_tensor(qkr[:, half + 2 + c, :], tc[:],
                                                td[:], ALU.add)
                qkrs[s] = qkr

            # ---- phase A2 per sample: stage2, attention, out-proj ----
            rc2 = {}
            for s in range(BL):
                sl = slice(s * S, (s + 1) * S)
                qkr, vloc = qkrs[s], vlocs[s]
                qk2 = qk2p.tile([128, 8, S], F16, tag="qk2")
                for m in range(8):
                    ps = b1.tile([128, S], F32, tag="b1")
                    base = 0 if m < 4 else 4
                    for c in range(2):
                        nc.tensor.matmul(ps[:],
                                         w2[:, 2 * c:2 * c + 2, m * 128:(m + 1) * 128],
                                         qkr[:, base + 2 * c:base + 2 * c + 2, :],
                                         start=(c == 0), stop=(c == 1),
                                         perf_mode=DR)
                    nc.vector.scalar_tensor_tensor(
                        qk2[:, m, :], ps[:], 1.0 / WHS, cq[:, m, :],
                        ALU.mult, ALU.add)
                # attention heads (paired: exp(h+1) hides under A@V(h))
                o_t = a4.tile([128, KC, S], F8, tag="ot", bufs=2)
                for h0 in range(0, H, 2):
                    ats = {}
                    for hh in (h0, h0 + 1):
                        rows = slice(64 * (hh % 2), 64 * (hh % 2) + 64)
                        at = a4.tile([128, KC, S], F8, tag="at", bufs=3)
                        for c in range(KC):
                            scp = b1.tile([128, S], F32, tag="b1")
                            nc.tensor.matmul(
                                scp[:],
                                qk2[rows, 4 + hh // 2, c * 128:(c + 1) * 128],
                                qk2[rows, hh // 2, :], start=True, stop=True)
                            nc.scalar.activation(at[:, c, :], scp[:], AF.Exp,
                                                 scale=float(1.0 / np.sqrt(DH)))
                        ats[hh] = at
                    for hh in (h0, h0 + 1):
                        rows = slice(64 * (hh % 2), 64 * (hh % 2) + 64)
                        at = ats[hh]
                        ov = b2.tile([72, S], F32, tag="b2")
                        for c in range(2):
                            nc.tensor.matmul(ov[:], vloc[:, 2 * c:2 * c + 2, hh, :],
                                             at[:, 2 * c:2 * c + 2, :],
                                             start=(c == 0), stop=(c == 1),
                                             perf_mode=DR)
                        # OS/denom = exp(ln OS - ln denom), broadcast via PE
                        lnd = scr.tile([1, S], F16, tag="lnd")
                        nc.scalar.activation(lnd[:], ov[64:65, :], AF.Ln)
                        rb = b2.tile([64, S], F32, tag="b2")
                        nc.tensor.matmul(rb[:], ones_row[:, 0:64], lnd[:],
                                         start=True, stop=True)
                        inv = invp.tile([64, S], F16, tag="inv")
                        nc.scalar.activation(inv[:], rb[:], AF.Exp, scale=-1.0,
                                             bias=ln_os[:])
                        nc.vector.tensor_tensor(o_t[rows, hh // 2, :],
                                                ov[0:64, :], inv[:], ALU.mult)
                # out-proj + residual (bias injected into PSUM, then descale)
                for m in range(KC):
                    ps = b1.tile([128, S], F32, tag="b1")
                    nc.tensor.matmul(ps[:], owb[:, m * 128:(m + 1) * 128],
                                     ones_rowS[:], start=True, stop=False)
                    for c in range(2):
                        nc.tensor.matmul(ps[:],
                                         ow[:, 2 * c:2 * c + 2, m * 128:(m + 1) * 128],
                                         o_t[:, 2 * c:2 * c + 2, :],
                                         start=False, stop=(c == 1),
                                         perf_mode=DR)
                    nc.vector.scalar_tensor_tensor(
                        x[:, m, sl], ps[:], 1.0 / WOS, x[:, m, sl],
                        ALU.mult, ALU.add)
                rc2[s] = ln_stats(x, s)

            # ---- phase B: FFN (l2 streams in once w1 slot is released) ----
            l2 = wpool.tile([128, FC, D], F8, tag="bigw")
            for kf in range(FC):
                nc.sync.dma_start(l2[:, kf, :], dram["el2T"][li][:, kf, :])
            for s in range(BL):
                sl = slice(s * S, (s + 1) * S)
                h2 = hp.tile([128, KC, S], F8, tag="h")
                ln_apply(rc2[s], x, s, h2, scaled=True)
                f2 = [b1.tile([128, S], F32, tag="b1", name=f"f2_{_m}")
                      for _m in range(KC)]
                for m in range(KC):     # inject WOS*l2b into the accumulators
                    nc.tensor.matmul(f2[m][:], l2b[:, m * 128:(m + 1) * 128],
                                     ones_rowS[:], start=True, stop=False)
                for jf in range(FC // 2):
                    rt2 = rtp.tile([128, 2, S], F8, tag="rt8")
                    for i in range(2):
                        kf = 2 * jf + i
                        f1 = b2.tile([128, S], F32, tag="b2")
                        for c in range(2):
                            nc.tensor.matmul(
                                f1[:],
                                l1[:, 2 * c:2 * c + 2, kf * 128:(kf + 1) * 128],
                                h2[:, 2 * c:2 * c + 2, :],
                                start=(c == 0), stop=(c == 1), perf_mode=DR)
                        # rt = OS*relu(z + l1b):  f1 = WHS*z, bias = OS*l1b
                        nc.scalar.activation(rt2[:, i, :], f1[:], AF.Relu,
                                             scale=OS / WHS,
                                             bias=l1b[:, kf:kf + 1])
                    for m in range(KC):
                        nc.tensor.matmul(f2[m][:],
                                         l2[:, 2 * jf:2 * jf + 2, m * 128:(m + 1) * 128],
                                         rt2[:, :, :], start=False,
                                         stop=(jf == FC // 2 - 1), perf_mode=DR)
                for m in range(KC):
                    nc.vector.scalar_tensor_tensor(
                        x[:, m, sl], f2[m][:], 1.0 / WOS, x[:, m, sl],
                        ALU.mult, ALU.add)
                rc1[s] = ln_stats(x, s)   # next layer (or the final LN)

        # ---------------- final encoder LN (in-place; affine folded) --------
        me = x
        for s in range(BL):
            ln_apply(rc1[s], x, s, x[:, :, s * S:(s + 1) * S])

        # ---------------- decoder layers ----------------
        for li in range(ld):
            dw = wpool.tile([128, KC, 3 * D], F16, tag="bigw")
            nc.sync.dma_start(dw[:], dram["dinT"][li])
            dwb = bpool.tile([128, 12], F32, tag="w1b")
            nc.sync.dma_start(dwb[:], dram["dinb"][li])
            dvbrow = bpool.tile([1, D], F16, tag="vbrow")
            nc.sync.dma_start(dvbrow[:], dram["dvb"][li])
            do = owpool.tile([128, KC, D], F16, tag="ow")
            nc.sync.dma_start(do[:], dram["dowT"][li])
            dob = bpool.tile([128, 4], F32, tag="owb")
            nc.sync.dma_start(dob[:], dram["dowb"][li])
            m1 = wpool.tile([128, KC, M], F16, tag="bigw")
            nc.sync.dma_start(m1[:], dram["dm1T"][li])
            m1b = bpool.tile([128, 16], F32, tag="l1b")
            nc.sync.dma_start(m1b[:], dram["dm1b"][li])
            m2b = bpool.tile([128, 4], F32, tag="l2b")
            nc.sync.dma_start(m2b[:], dram["dm2b"][li])

            vb_ps = b1.tile([128, D], F32, tag="b1")
            nc.tensor.matmul(vb_ps[:], ones_row[:], dvbrow[:], start=True, stop=True)
            vb_bc = vbp.tile([128, D], F16, tag="vb_bc")
            nc.scalar.activation(vb_bc[:], vb_ps[:], AF.Copy)

            # LN(p) -> q_ln ; Q projection (all samples at once, N=BL)
            q_ln = smalls.tile([128, KC, BL], F16, tag="q_ln")
            ln_small(p, BL, q_ln)
            qps = b1.tile([128, KC, BL], F32, tag="b1")
            for m in range(KC):
                for k in range(KC):
                    nc.tensor.matmul(qps[:, m, :],
                                     dw[:, k, m * 128:(m + 1) * 128],
                                     q_ln[:, k, :], start=(k == 0),
                                     stop=(k == KC - 1))
            q_sb = smalls.tile([128, KC, BL], F16, tag="q_sb")
            for m in range(KC):
                nc.scalar.activation(q_sb[:, m, :], qps[:, m, :], AF.Identity,
                                     bias=dwb[:, m:m + 1])
            o_d = smalls.tile([128, KC, BL], F16, tag="o_d")
            for s in range(BL):
                sl = slice(s * S, (s + 1) * S)
                # K (feature-major) and V' (token-major) over morph_enc
                k_sb = a4.tile([128, KC, S], F16, tag="at", bufs=3)
                for m in range(KC):
                    ps = b1.tile([128, S], F32, tag="b1")
                    for k in range(KC):
                        nc.tensor.matmul(
                            ps[:], dw[:, k, D + m * 128:D + (m + 1) * 128],
                            me[:, k, sl], start=(k == 0), stop=(k == KC - 1))
                    nc.scalar.activation(k_sb[:, m, :], ps[:], AF.Identity,
                                         bias=dwb[:, 4 + m:5 + m])
                vloc = vp.tile([128, KC, 8, 65], F16, tag="vloc")
                for t in range(KC):
                    nc.vector.tensor_copy(vloc[:, t, :, 64], ones8[:])
                for t in range(KC):
                    ps = b1.tile([128, S], F32, tag="b1")
                    for k in range(KC):
                        nc.tensor.matmul(
                            ps[:], me[:, k, s * S + t * 128:s * S + (t + 1) * 128],
                            dw[:, k, 2 * D:3 * D],
                            start=(k == 0), stop=(k == KC - 1))
                    nc.vector.tensor_tensor(
                        vloc[:, t, :, 0:64],
                        ps[:].rearrange("p (h d) -> p h d", h=H),
                        vb_bc[:].rearrange("p (h d) -> p h d", h=H), ALU.add)
                scp = b1.tile([128, KC, H], F32, tag="b1")
                for hh in range(H):
                    rows = slice(64 * (hh % 2), 64 * (hh % 2) + 64)
                    for c in range(KC):
                        nc.tensor.matmul(
                            scp[:, c, hh:hh + 1],
                            k_sb[rows, hh // 2, c * 128:(c + 1) * 128],
                            q_sb[rows, hh // 2, s:s + 1],
                            start=True, stop=True)
                at = smalls.tile([128, KC, H], F16, tag="at_d")
                nc.scalar.activation(at[:], scp[:], AF.Exp,
                                     scale=float(1.0 / np.sqrt(DH)))
                ov = b2.tile([65, H], F32, tag="b2")
                for hh in range(H):
                    for c in range(KC):
                        nc.tensor.matmul(ov[:, hh:hh + 1], vloc[:, c, hh, :],
                                         at[:, c, hh:hh + 1],
                                         start=(c == 0), stop=(c == KC - 1))
                # 1/denom = exp(-ln denom)
                lnd = scr.tile([1, H], F16, tag="lnd")
                nc.scalar.activation(lnd[:], ov[64:65, :], AF.Ln)
                rb = b2.tile([64, H], F32, tag="b2")
                nc.tensor.matmul(rb[:], ones_row[:, 0:64], lnd[:],
                                 start=True, stop=True)
                inv = invp.tile([64, H], F16, tag="inv_d")
                nc.scalar.activation(inv[:], rb[:], AF.Exp, scale=-1.0)
                for hh in range(H):
                    rows = slice(64 * (hh % 2), 64 * (hh % 2) + 64)
                    nc.vector.tensor_tensor(o_d[rows, hh // 2, s:s + 1],
                                            ov[0:64, hh:hh + 1],
                                            inv[:, hh:hh + 1], ALU.mult)
            # out-proj + residual into p
            ops = b1.tile([128, KC, BL], F32, tag="b1")
            for m in range(KC):
                for k in range(KC):
                    nc.tensor.matmul(ops[:, m, :],
                                     do[:, k, m * 128:(m + 1) * 128],
                                     o_d[:, k, :], start=(k == 0),
                                     stop=(k == KC - 1))
            for m in range(KC):
                nc.vector.scalar_tensor_tensor(
                    p[:, m, :], ops[:, m, :], dob[:, m:m + 1], p[:, m, :],
                    ALU.add, ALU.add)
            # FFN on p (m2 streams in chunked once dw releases its slot)
            m2 = wpool.tile([128, MC, D], F16, tag="bigw")
            for kf in range(MC):
                nc.sync.dma_start(m2[:, kf, :], dram["dm2T"][li][:, kf, :])
            h2d = smalls.tile([128, KC, BL], F16, tag="q_ln")
            ln_small(p, BL, h2d)
            mh = smalls.tile([128, MC, BL], F16, tag="mh")
            for mm_ in range(MC):
                ps = b1.tile([128, BL], F32, tag="b1")
                for k in range(KC):
                    nc.tensor.matmul(ps[:], m1[:, k, mm_ * 128:(mm_ + 1) * 128],
                                     h2d[:, k, :], start=(k == 0),
                                     stop=(k == KC - 1))
                nc.scalar.activation(mh[:, mm_, :], ps[:], AF.Relu,
                                     bias=m1b[:, mm_:mm_ + 1])
            m2ps = b1.tile([128, KC, BL], F32, tag="b1")
            for m in range(KC):
                for kf in range(MC):
                    nc.tensor.matmul(m2ps[:, m, :],
                                     m2[:, kf, m * 128:(m + 1) * 128],
                                     mh[:, kf, :], start=(kf == 0),
                                     stop=(kf == MC - 1))
            for m in range(KC):
                nc.vector.scalar_tensor_tensor(
                    p[:, m, :], m2ps[:, m, :], m2b[:, m:m + 1], p[:, m, :],
                    ALU.add, ALU.add)

        # ---------------- head ----------------
        hw = smalls.tile([128, KC], F16, tag="hw")
        hb = smalls.tile([1, 1], F32, tag="hb")
        nc.sync.dma_start(hw[:], dram["hwT"][:])
        nc.sync.dma_start(hb[:], dram["hb"][:])
        hg = smalls.tile([128, KC, BL], F16, tag="q_ln")
        ln_small(p, BL, hg)
        hps = b2.tile([1, BL], F32, tag="b2")
        for k in range(KC):
            nc.tensor.matmul(hps[:], hw[:, k:k + 1], hg[:, k, :],
                             start=(k == 0), stop=(k == KC - 1))
        y_sb = smalls.tile([1, BL], F32, tag="y_sb")
        nc.scalar.activation(y_sb[:], hps[:], AF.Sigmoid, bias=hb[:])
        nc.sync.dma_start(y_dram[:], y_sb[:])


# ----------------------------------------------------------------------------
# entry point
# ----------------------------------------------------------------------------

_NC_CACHE = {}


def kernel(**inputs):
    return _run(inputs, LE, LD)


def _run(inputs, le, ld, trace=False):
    w = prep_weights(inputs, le, ld)
    morph = np.asarray(inputs["morph"], np.float32)
    pose = np.asarray(inputs["pose"], np.float32)
    in_maps = []
    for c in range(NCORES):
        im = dict(w)
        mo = morph[c * BL:(c + 1) * BL]                 # [BL, S, 3]
        im["morphT"] = np.ascontiguousarray(
            mo.transpose(2, 0, 1).reshape(3, T)).astype(np.float16)
        im["poseT"] = np.ascontiguousarray(
            pose[c * BL:(c + 1) * BL].T).astype(np.float16)
        in_maps.append(im)

    if ("nc", le, ld) not in _NC_CACHE:
        _NC_CACHE[("nc", le, ld)] = build(le, ld)
    nc = _NC_CACHE[("nc", le, ld)]
    res = run_bass_kernel_spmd(nc, in_maps, core_ids=list(range(NCORES)),
                               trace=trace)
    out = np.zeros((B, 1), np.float32)
    for c in range(NCORES):
        out[c * BL:(c + 1) * BL, 0] = res.results[c]["y"][0]
    if trace:
        return out, res
    return out


# revision 47
# speedup vs baseline: 1.8371x; 1.0290x over previous
"""Trainium2 Bass kernel for nn_ReachabilityClassifierTransformer.

Data-parallel over batch: 16 samples / 8 cores = 2 samples per core.
Each core runs the full network (6-layer encoder + 4-layer decoder + head)
on its 2 samples. No collectives.

v2 (fp16 pipeline):
  - All matmul operands fp16 (stationary weights get Fast-Weight-Load; DVE
    elementwise ops run in 2x mode; DMA traffic halved). PSUM stays fp32.
  - V path folded on host: V = h @ (Wv2 Wv1)^T + bv  (no rope between the
    two V projections, so the double-projection quirk collapses).
  - Stage-1 q/k biases folded through rope into per-position bias tensors
    C_q/C_k = W2 @ rope(b1) + b2, added at the stage-2 PSUM copy (rope is
    linear, rotation depends only on position).
  - No Sqrt / no DVE reciprocal anywhere: LN rsqrt = exp(-0.5 ln(v+eps)),
    softmax 1/denom = exp(-ln denom) broadcast via PE.  ln/exp/copy/relu/
    square all live in one activation table set -> no table switches.
  - Per-layer phase order interleaves the two samples so rope (DVE) and
    softmax exp (Act) hide under the other sample's matmuls.

Device layout conventions (per core):
  - Activations FEATURE-MAJOR in SBUF: tile [128, KC, T] holds X.T.
  - Weights pre-transposed on host to [in_feat, out_feat], laid out
    [128, KC_in, O] (partition = in-feature % 128).
  - matmul(out_psum[M,N], lhsT=[K,M], rhs=[K,N]) computes lhsT.T @ rhs.
  - Encoder stage-1 q,k output features are de-interleaved (even feats then
    odd feats) via host-side column permutation, so RoPE becomes contiguous
    block ops; the roped result is in natural (concatenated) order.
  - Softmax: scores computed transposed (S.T = K_h @ Q_h.T per chunk),
    exp'd without max subtraction (|scores/8| < 1 for this model), and the
    denominator comes free from a ones-column appended to V.
"""
import functools

import numpy as np

import concourse.bass as bass
import concourse.mybir as mybir
import concourse.tile as tile
from concourse import bacc
from concourse.bass_utils import run_bass_kernel_spmd


def _patch_act_tables():
    """Constrain exp/ln to the one table set that contains both.

    The act-table-load pass maps each activation function to a set
    independently (exp -> exp_and_others, ln -> natural_log), so a kernel
    that interleaves exp and ln reloads tables on every transition
    (~1.3us each).  natural_log_exp_and_others contains exp AND ln (plus
    copy/identity/relu/square), so restricting exp/ln to that set makes
    every load resolve there; set ids/order are preserved so the emitted
    act_func_set_id still indexes the real act_info.json.
    """
    import concourse.hw_specs as hw_specs
    if getattr(hw_specs, "_ant_act_tables_patched", False):
        return
    orig = hw_specs.get_activation_tables

    @functools.cache
    def patched(module_arch):
        t = orig(module_arch)
        keep = "natural_log_exp_and_others"
        if keep not in t:
            return t
        drop = {mybir.ActivationFunctionType.Exp, mybir.ActivationFunctionType.Ln}
        return {name: (fns if name == keep else fns - drop)
                for name, fns in t.items()}

    hw_specs._ant_act_tables_patched = True
    hw_specs.get_activation_tables = patched
    import sys
    for modname in ("concourse.bacc", "concourse.bass_interp"):
        mod = sys.modules.get(modname)
        if mod is not None and hasattr(mod, "get_activation_tables"):
            mod.get_activation_tables = patched


_patch_act_tables()

AF = mybir.ActivationFunctionType
ALU = mybir.AluOpType
F32 = mybir.dt.float32
F16 = mybir.dt.float16
F8 = mybir.dt.float8e4
F8NP = mybir.dt.np(F8)
DR = mybir.MatmulPerfMode.DoubleRow
F8MAX = 240.0          # TRN fp8e4 saturation (not OCP's 448)
WS = 64.0              # fp8 FFN weight scale
HS = 16.0              # fp8 FFN input activation scale
OS = 32.0              # fp8 FFN hidden scale
WHS = WS * HS          # l1 psum descale
WOS = WS * OS          # l2 psum descale

B, S, D, FF, H, LE, LD, M = 16, 512, 512, 2048, 8, 6, 4, 2048
ROPE_BASE = 10000.0
LN_EPS = 1e-5
NCORES = 8
BL = B // NCORES          # 2 samples per core
T = BL * S                # 1024 tokens per core
KC = D // 128             # 4 feature chunks
FC = FF // 128            # 16
MC = M // 128             # 16
DH = D // H               # 64


# ----------------------------------------------------------------------------
# host-side helpers
# ----------------------------------------------------------------------------

def _chunked(wT):
    """[Din, O] -> [128, Din//128, O] contiguous fp16."""
    Din, O = wT.shape
    return np.ascontiguousarray(
        wT.reshape(Din // 128, 128, O).transpose(1, 0, 2)).astype(np.float16)


def _bias_cols(b):
    """[O] -> [128, O//128]  (column per 128-chunk), fp32."""
    O = b.shape[0]
    return np.ascontiguousarray(b.reshape(O // 128, 128).T).astype(np.float32)


def _chunked8(wT, scale=WS):
    """[Din, O] -> [128, Din//128, O] contiguous fp8e4, pre-scaled."""
    Din, O = wT.shape
    a = np.clip(wT * scale, -F8MAX, F8MAX)
    return np.ascontiguousarray(
        a.reshape(Din // 128, 128, O).transpose(1, 0, 2)).astype(F8NP)


_DEINT = np.concatenate([np.arange(0, D, 2), np.arange(1, D, 2)])  # de-interleave


def prep_weights(inp, le=LE, ld=LD):
    """Host-side weight prep -> dict of arrays shared by all cores."""
    out = {}
    g = {k: np.asarray(v, np.float64) for k, v in inp.items()}

    out["mpwT"] = np.ascontiguousarray(g["morph_proj_w"].T).astype(np.float16)
    out["mpb"] = _bias_cols(g["morph_proj_b"])                     # [128, 4]
    out["ppwT"] = np.ascontiguousarray(g["pose_proj_w"].T).astype(np.float16)
    out["ppb"] = _bias_cols(g["pose_proj_b"])

    # rope grids, de-interleaved frequency order: [128, 2, 512] fp16
    freq = 1.0 / ROPE_BASE ** (np.arange(0, D, 2, dtype=np.float64) / D)
    ang = np.outer(np.arange(S, dtype=np.float64), freq)           # [512, 256]
    cosT = np.cos(ang).T                                           # [256, S]
    sinT = np.sin(ang).T
    out["gridc"] = _chunked(cosT.reshape(256, S))
    out["grids"] = _chunked(sinT.reshape(256, S))

    e_w1, e_w2, e_wv, e_cqk, e_vb = [], [], [], [], []
    e_ow, e_owb, e_l1, e_l1b, e_l2, e_l2b = [], [], [], [], [], []
    for i in range(le):
        w1 = g["enc_in_w"][i] * g["enc_n1_g"][i][None, :]          # fold n1 g
        b1 = g["enc_in_b"][i] + g["enc_in_w"][i] @ g["enc_n1_b"][i]
        # stage-1 q,k only, de-interleaved output columns
        perm = np.concatenate([_DEINT, D + _DEINT])
        e_w1.append(_chunked(np.ascontiguousarray(w1[perm].T)))    # [128,4,1024]
        # stage-2 q,k (natural order, raw weights - the faithful quirk)
        w2 = g["enc_in_w"][i][: 2 * D]                             # Wq;Wk
        e_w2.append(_chunked(np.ascontiguousarray(w2.T)))          # [128,4,1024]
        # stage-2 bias tensors: C = W2 @ rope(b1) + b2   [512, S] each
        bq = b1[:D][_DEINT]                                        # [even; odd]
        bk = b1[D:2 * D][_DEINT]
        rb_q = np.concatenate([bq[:256, None] * cosT - bq[256:, None] * sinT,
                               bq[:256, None] * sinT + bq[256:, None] * cosT])
        rb_k = np.concatenate([bk[:256, None] * cosT - bk[256:, None] * sinT,
                               bk[:256, None] * sinT + bk[256:, None] * cosT])
        Cq = g["enc_in_w"][i][:D] @ rb_q + g["enc_in_b"][i][:D][:, None]
        Ck = g["enc_in_w"][i][D:2 * D] @ rb_k \
            + g["enc_in_b"][i][D:2 * D][:, None]
        C = np.concatenate([Cq, Ck], axis=0)                       # [1024, S]
        e_cqk.append(_chunked(C))                                  # [128,8,S]
        # V folded: V = h @ (Wv2 Wv1_f).T + (Wv2 bv1_f + bv2)
        Wv1f = w1[2 * D:]
        bv1f = b1[2 * D:]
        Wv2 = g["enc_in_w"][i][2 * D:]
        bv2 = g["enc_in_b"][i][2 * D:]
        e_wv.append(_chunked(np.ascontiguousarray((Wv2 @ Wv1f).T)))
        e_vb.append((Wv2 @ bv1f + bv2)[None, :].astype(np.float16))  # [1,512]
        e_ow.append(_chunked(np.ascontiguousarray(g["enc_out_w"][i].T)))
        e_owb.append(_bias_cols(g["enc_out_b"][i]))
        l1 = g["enc_l1_w"][i] * g["enc_n2_g"][i][None, :]
        l1b = g["enc_l1_b"][i] + g["enc_l1_w"][i] @ g["enc_n2_b"][i]
        e_l1.append(_chunked8(np.ascontiguousarray(l1.T)))         # [128,4,2048]
        e_l1b.append(_bias_cols(OS * l1b))                         # [128,16]
        e_l2.append(_chunked8(np.ascontiguousarray(g["enc_l2_w"][i].T)))
        e_l2b.append((WOS * g["enc_l2_b"][i])[None, :].astype(np.float16))
    out["ew1T"] = np.stack(e_w1) if le else np.zeros((0, 128, KC, 2 * D), np.float16)
    out["ew2T"] = np.stack(e_w2) if le else np.zeros((0, 128, KC, 2 * D), np.float16)
    out["ewvT"] = np.stack(e_wv) if le else np.zeros((0, 128, KC, D), np.float16)
    out["ecqk"] = np.stack(e_cqk) if le else np.zeros((0, 128, 8, S), np.float16)
    out["evb"] = np.stack(e_vb) if le else np.zeros((0, 1, D), np.float16)
    out["eowT"] = np.stack(e_ow) if le else np.zeros((0, 128, KC, D), np.float16)
    out["eowb"] = np.stack(e_owb) if le else np.zeros((0, 128, 4), np.float32)
    out["el1T"] = np.stack(e_l1) if le else np.zeros((0, 128, KC, FF), F8NP)
    out["el1b"] = np.stack(e_l1b) if le else np.zeros((0, 128, 16), np.float32)
    out["el2T"] = np.stack(e_l2) if le else np.zeros((0, 128, FC, D), F8NP)
    out["el2b"] = np.stack(e_l2b) if le else np.zeros((0, 1, D), np.float16)

    d_in, d_inb, d_vb, d_ow, d_owb = [], [], [], [], []
    d_m1, d_m1b, d_m2, d_m2b = [], [], [], []
    for i in range(ld):
        w = g["dec_in_w"][i].copy()
        b = g["dec_in_b"][i].copy()
        w[:D] = w[:D] * g["dec_n1_g"][i][None, :]                  # Wq <- dec_n1
        b[:D] = b[:D] + g["dec_in_w"][i][:D] @ g["dec_n1_b"][i]
        w[D:] = w[D:] * g["enc_final_g"][None, :]                  # Wk,Wv <- enc_final
        b[D:] = b[D:] + g["dec_in_w"][i][D:] @ g["enc_final_b"]
        d_in.append(_chunked(np.ascontiguousarray(w.T)))           # [128,4,1536]
        d_inb.append(_bias_cols(b))
        d_vb.append(b[2 * D:][None, :].astype(np.float16))         # [1,512]
        d_ow.append(_chunked(np.ascontiguousarray(g["dec_out_w"][i].T)))
        d_owb.append(_bias_cols(g["dec_out_b"][i]))
        m1 = g["dec_m1_w"][i] * g["dec_n2_g"][i][None, :]
        m1b = g["dec_m1_b"][i] + g["dec_m1_w"][i] @ g["dec_n2_b"][i]
        d_m1.append(_chunked(np.ascontiguousarray(m1.T)))          # [128,4,2048]
        d_m1b.append(_bias_cols(m1b))
        d_m2.append(_chunked(np.ascontiguousarray(g["dec_m2_w"][i].T)))
        d_m2b.append(_bias_cols(g["dec_m2_b"][i]))
    out["dinT"] = np.stack(d_in) if ld else np.zeros((0, 128, KC, 3 * D), np.float16)
    out["dinb"] = np.stack(d_inb) if ld else np.zeros((0, 128, 12), np.float32)
    out["dvb"] = np.stack(d_vb) if ld else np.zeros((0, 1, D), np.float16)
    out["dowT"] = np.stack(d_ow) if ld else np.zeros((0, 128, KC, D), np.float16)
    out["dowb"] = np.stack(d_owb) if ld else np.zeros((0, 128, 4), np.float32)
    out["dm1T"] = np.stack(d_m1) if ld else np.zeros((0, 128, KC, M), np.float16)
    out["dm1b"] = np.stack(d_m1b) if ld else np.zeros((0, 128, 16), np.float32)
    out["dm2T"] = np.stack(d_m2) if ld else np.zeros((0, 128, MC, D), np.float16)
    out["dm2b"] = np.stack(d_m2b) if ld else np.zeros((0, 128, 4), np.float32)

    hw = (g["head_w"] * g["head_g"][None, :])[0]                   # [512]
    out["hwT"] = _bias_cols(hw).astype(np.float16)                 # [128, 4]
    out["hb"] = (g["head_bias"] + g["head_w"] @ g["head_b"]).reshape(1, 1).astype(np.float32)
    return out


# ----------------------------------------------------------------------------
# device program
# ----------------------------------------------------------------------------

def build(le=LE, ld=LD):
    nc = bacc.Bacc(None, target_bir_lowering=False)

    dram = {}

    def din(name, shape, dt=F16):
        dram[name] = nc.dram_tensor(name, list(shape), dt, kind="ExternalInput")
        return dram[name]

    # shared weights
    din("mpwT", [3, 512]); din("mpb", [128, 4], F32)
    din("ppwT", [9, 512]); din("ppb", [128, 4], F32)
    din("gridc", [128, 2, S]); din("grids", [128, 2, S])
    din("ew1T", [le, 128, KC, 2 * D])
    din("ew2T", [le, 128, KC, 2 * D])
    din("ewvT", [le, 128, KC, D])
    din("ecqk", [le, 128, 8, S])
    din("evb", [le, 1, D])
    din("eowT", [le, 128, KC, D]); din("eowb", [le, 128, 4], F32)
    din("el1T", [le, 128, KC, FF], F8); din("el1b", [le, 128, 16], F32)
    din("el2T", [le, 128, FC, D], F8); din("el2b", [le, 1, D])
    din("dinT", [ld, 128, KC, 3 * D]); din("dinb", [ld, 128, 12], F32)
    din("dvb", [ld, 1, D])
    din("dowT", [ld, 128, KC, D]); din("dowb", [ld, 128, 4], F32)
    din("dm1T", [ld, 128, KC, M]); din("dm1b", [ld, 128, 16], F32)
    din("dm2T", [ld, 128, MC, D]); din("dm2b", [ld, 128, 4], F32)
    din("hwT", [128, KC]); din("hb", [1, 1], F32)
    # per-core inputs
    din("morphT", [3, T])
    din("poseT", [9, BL])
    y = nc.dram_tensor("y", [1, BL], F32, kind="ExternalOutput")

    with tile.TileContext(nc) as tc:
        _build_body(nc, tc, dram, y, le, ld)
    nc.compile()
    return nc


def _build_body(nc, tc, dram, y_dram, le, ld):
    import contextlib
    ctx = contextlib.ExitStack()
    with ctx:
        ctx.enter_context(nc.allow_low_precision(
            reason="fp16 matmul operands / activations are intentional"))
        persist = ctx.enter_context(tc.tile_pool(name="persist", bufs=1))
        wpool = ctx.enter_context(tc.tile_pool(name="wpool", bufs=2))
        w2pool = ctx.enter_context(tc.tile_pool(name="w2pool", bufs=1))
        owpool = ctx.enter_context(tc.tile_pool(name="owpool", bufs=1))
        vwpool = ctx.enter_context(tc.tile_pool(name="vwpool", bufs=1))
        cqpool = ctx.enter_context(tc.tile_pool(name="cqpool", bufs=1))
        bpool = ctx.enter_context(tc.tile_pool(name="bpool", bufs=2))
        a4 = ctx.enter_context(tc.tile_pool(name="a4", bufs=4))
        hp = ctx.enter_context(tc.tile_pool(name="hp", bufs=2))
        a8 = ctx.enter_context(tc.tile_pool(name="a8", bufs=3))
        qk2p = ctx.enter_context(tc.tile_pool(name="qk2p", bufs=2))
        vp = ctx.enter_context(tc.tile_pool(name="vp", bufs=2))
        rtp = ctx.enter_context(tc.tile_pool(name="rtp", bufs=3))
        vbp = ctx.enter_context(tc.tile_pool(name="vbp", bufs=1))
        scr = ctx.enter_context(tc.tile_pool(name="scr", bufs=3))
        invp = ctx.enter_context(tc.tile_pool(name="invp", bufs=2))
        rcp = ctx.enter_context(tc.tile_pool(name="rcp", bufs=2))
        smalls = ctx.enter_context(tc.tile_pool(name="smalls", bufs=2))
        b1 = ctx.enter_context(tc.tile_pool(name="b1", bufs=4, space="PSUM"))
        b2 = ctx.enter_context(tc.tile_pool(name="b2", bufs=4, space="PSUM"))

        # ---------------- persistent tiles ----------------
        x = persist.tile([128, KC, T], F16)           # residual stream (X.T)
        gridc = persist.tile([128, 2, S], F16)
        grids = persist.tile([128, 2, S], F16)
        ones128 = persist.tile([128, 1], F16)
        ones_row = persist.tile([1, 128], F16)
        ones_rowS = persist.tile([1, S], F16)         # bias-inject moving row
        hs_row = persist.tile([1, 128], F16)          # HS-scaled broadcast row
        ones8 = persist.tile([128, 8], F16)
        eps_t = persist.tile([1, 1], F32)
        p = persist.tile([128, KC, BL], F16)          # decoder latent p.T
        nc.sync.dma_start(gridc[:], dram["gridc"][:])
        nc.sync.dma_start(grids[:], dram["grids"][:])
        stage_f16 = rtp.tile([128, 128], F16, tag="rt")
        nc.vector.memset(stage_f16[:], 1.0)
        nc.vector.tensor_copy(ones128[:], stage_f16[:, 0:1])
        nc.vector.tensor_copy(ones_row[:], stage_f16[0:1, :])
        nc.vector.tensor_copy(ones8[:], stage_f16[:, 0:8])
        nc.vector.memset(hs_row[:], HS)
        nc.vector.memset(ones_rowS[:], 1.0)
        nc.vector.memset(eps_t[:], LN_EPS)

        def ln_small(x_tile, n_tok, h_out):
            """h_out = LayerNorm_features(x_tile) for tiny n_tok (decoder)."""
            sq = smalls.tile([128, KC, n_tok], F16, tag="sq_d")
            for k in range(KC):
                nc.vector.tensor_tensor(sq[:, k, :], x_tile[:, k, :],
                                        x_tile[:, k, :], ALU.mult)
            sum_ps = b2.tile([1, n_tok], F32, tag="b2")
            sq_ps = b2.tile([1, n_tok], F32, tag="b2")
            for k in range(KC):
                nc.tensor.matmul(sum_ps[:], ones128[:], x_tile[:, k, :],
                                 start=(k == 0), stop=(k == KC - 1))
            for k in range(KC):
                nc.tensor.matmul(sq_ps[:], ones128[:], sq[:, k, :],
                                 start=(k == 0), stop=(k == KC - 1))
            ms = scr.tile([1, n_tok], F32, tag="scr")
            t2 = scr.tile([1, n_tok], F32, tag="scr")
            rc = scr.tile([1, 2, n_tok], F16, tag="scr_rc")
            nc.scalar.activation(ms[:], sum_ps[:], AF.Copy, scale=1.0 / D)
            nc.vector.tensor_tensor(t2[:], ms[:], ms[:], ALU.mult)      # m^2
            nc.vector.scalar_tensor_tensor(
                t2[:], sq_ps[:], 1.0 / D, t2[:], ALU.mult, ALU.subtract)
            # r = exp(-0.5 ln(var + eps))
            nc.scalar.activation(t2[:], t2[:], AF.Ln, bias=eps_t[:])
            nc.scalar.activation(rc[:, 0, :], t2[:], AF.Exp, scale=-0.5)
            nc.vector.scalar_tensor_tensor(
                rc[:, 1, :], ms[:], -1.0, rc[:, 0, :], ALU.mult, ALU.mult)
            r_ps = b1.tile([128, n_tok], F32, tag="b1")
            c_ps = b1.tile([128, n_tok], F32, tag="b1")
            nc.tensor.matmul(r_ps[:], ones_row[:], rc[:, 0, :],
                             start=True, stop=True)
            nc.tensor.matmul(c_ps[:], ones_row[:], rc[:, 1, :],
                             start=True, stop=True)
            for k in range(KC):
                nc.vector.tensor_tensor(h_out[:, k, :], x_tile[:, k, :],
                                        r_ps[:], ALU.mult)
                nc.vector.tensor_tensor(h_out[:, k, :], h_out[:, k, :],
                                        c_ps[:], ALU.add)

        def ln_stats(x_tile, s):
            """Per-sample LN stats -> rc [1, 2, S] fp16 (r, c)."""
            sl = slice(s * S, (s + 1) * S)
            sq = a4.tile([128, KC, S], F16, tag="sq", bufs=2)
            nc.vector.tensor_tensor(sq[:], x_tile[:, :, sl], x_tile[:, :, sl],
                                    ALU.mult)
            sum_ps = b2.tile([1, S], F32, tag="b2")
            sq_ps = b2.tile([1, S], F32, tag="b2")
            for k in range(KC):
                nc.tensor.matmul(sum_ps[:], ones128[:], x_tile[:, k, sl],
                                 start=(k == 0), stop=(k == KC - 1))
            for k in range(KC):
                nc.tensor.matmul(sq_ps[:], ones128[:], sq[:, k, :],
                                 start=(k == 0), stop=(k == KC - 1))
            ms = scr.tile([1, S], F32, tag="scr")
            t2 = scr.tile([1, S], F32, tag="scr")
            rc = rcp.tile([1, 2, S], F16, tag="rc", bufs=4)
            nc.scalar.activation(ms[:], sum_ps[:], AF.Copy, scale=1.0 / D)
            nc.vector.tensor_tensor(t2[:], ms[:], ms[:], ALU.mult)
            nc.vector.scalar_tensor_tensor(
                t2[:], sq_ps[:], 1.0 / D, t2[:], ALU.mult, ALU.subtract)
            nc.scalar.activation(t2[:], t2[:], AF.Ln, bias=eps_t[:])
            nc.scalar.activation(rc[:, 0, :], t2[:], AF.Exp, scale=-0.5)
            nc.vector.scalar_tensor_tensor(
                rc[:, 1, :], ms[:], -1.0, rc[:, 0, :], ALU.mult, ALU.mult)
            return rc

        def ln_apply(rc, x_tile, s, h_out, scaled=False):
            """h_out = (x[:, :, s] * r + c) * (HS if scaled else 1)."""
            sl = slice(s * S, (s + 1) * S)
            row = hs_row if scaled else ones_row
            r_ps = b1.tile([128, S], F32, tag="b1")
            c_ps = b1.tile([128, S], F32, tag="b1")
            nc.tensor.matmul(r_ps[:], row[:], rc[:, 0, :],
                             start=True, stop=True)
            nc.tensor.matmul(c_ps[:], row[:], rc[:, 1, :],
                             start=True, stop=True)
            rcb = rcp.tile([128, 2, S], F16, tag="rcb")
            nc.scalar.activation(rcb[:, 0, :], r_ps[:], AF.Copy)
            nc.scalar.activation(rcb[:, 1, :], c_ps[:], AF.Copy)
            for k in range(KC):
                tmp = rtp.tile([128, S], F16, tag="rt")
                nc.vector.tensor_tensor(tmp[:], x_tile[:, k, sl],
                                        rcb[:, 0, :], ALU.mult)
                nc.vector.tensor_tensor(h_out[:, k, :], tmp[:],
                                        rcb[:, 1, :], ALU.add)

        # ---------------- morph projection -> x ----------------
        morpht = rtp.tile([3, T], F16, tag="morph", bufs=1)
        nc.sync.dma_start(morpht[:], dram["morphT"][:])
        mpw = rtp.tile([3, 512], F16, tag="rt")
        mpb = rtp.tile([128, 4], F32, tag="rtb")
        nc.sync.dma_start(mpw[:], dram["mpwT"][:])
        nc.sync.dma_start(mpb[:], dram["mpb"][:])
        rc1 = {}
        for s in range(BL):
            for m in range(KC):
                ps = b1.tile([128, S], F32, tag="b1")
                nc.tensor.matmul(ps[:], mpw[:, m * 128:(m + 1) * 128],
                                 morpht[:, s * S:(s + 1) * S], start=True, stop=True)
                nc.scalar.activation(x[:, m, s * S:(s + 1) * S], ps[:], AF.Relu,
                                     bias=mpb[:, m:m + 1])
            rc1[s] = ln_stats(x, s)

        # ---------------- pose projection -> p ----------------
        poset = rtp.tile([9, BL], F16, tag="rt")
        ppw = rtp.tile([9, 512], F16, tag="rt")
        ppb = rtp.tile([128, 4], F32, tag="rtb")
        nc.sync.dma_start(poset[:], dram["poseT"][:])
        nc.sync.dma_start(ppw[:], dram["ppwT"][:])
        nc.sync.dma_start(ppb[:], dram["ppb"][:])
        pps = b1.tile([128, KC, BL], F32, tag="b1")
        for m in range(KC):
            nc.tensor.matmul(pps[:, m, :], ppw[:, m * 128:(m + 1) * 128],
                             poset[:], start=True, stop=True)
        for m in range(KC):
            nc.scalar.activation(p[:, m, :], pps[:, m, :], AF.Relu,
                                 bias=ppb[:, m:m + 1])

        # ---------------- encoder layers ----------------
        for li in range(le):
            w1 = wpool.tile([128, KC, 2 * D], F16, tag="bigw")
            nc.sync.dma_start(w1[:], dram["ew1T"][li])
            w2 = w2pool.tile([128, KC, 2 * D], F16, tag="w2")
            nc.sync.dma_start(w2[:], dram["ew2T"][li])
            wv = vwpool.tile([128, KC, D], F16, tag="wv")
            nc.sync.dma_start(wv[:], dram["ewvT"][li])
            cq = cqpool.tile([128, 8, S], F16, tag="cq")
            nc.sync.dma_start(cq[:], dram["ecqk"][li])
            vbrow = bpool.tile([1, D], F16, tag="vbrow")
            nc.sync.dma_start(vbrow[:], dram["evb"][li])
            ow = owpool.tile([128, KC, D], F16, tag="ow")
            nc.sync.dma_start(ow[:], dram["eowT"][li])
            owb = bpool.tile([128, 4], F32, tag="owb")
            nc.sync.dma_start(owb[:], dram["eowb"][li])
            l1 = wpool.tile([128, KC, FF], F8, tag="bigw")
            nc.sync.dma_start(l1[:], dram["el1T"][li])
            l1b = bpool.tile([128, 16], F32, tag="l1b")
            nc.sync.dma_start(l1b[:], dram["el1b"][li])
            l2b = bpool.tile([1, D], F16, tag="l2br")
            nc.sync.dma_start(l2b[:], dram["el2b"][li])

            # v-bias broadcast [128, 512], once per layer
            vb_ps = b1.tile([128, D], F32, tag="b1")
            nc.tensor.matmul(vb_ps[:], ones_row[:], vbrow[:], start=True, stop=True)
            vb_bc = vbp.tile([128, D], F16, tag="vb_bc")
            nc.scalar.activation(vb_bc[:], vb_ps[:], AF.Copy)

            qkrs, vlocs = {}, {}
            # ---- phase A1 per sample: LN1-apply, stage1 q,k; V; rope ----
            # (rc1 stats were emitted at the end of the previous phase so the
            #  scalar chain overlapped that phase's matmuls)
            for s in range(BL):
                h = hp.tile([128, KC, S], F16, tag="h")
                ln_apply(rc1[s], x, s, h)
                qkv1 = a8.tile([128, 8, S], F16, tag="a8")
                for m in range(8):
                    ps = b1.tile([128, S], F32, tag="b1")
                    for k in range(KC):
                        nc.tensor.matmul(ps[:], w1[:, k, m * 128:(m + 1) * 128],
                                         h[:, k, :], start=(k == 0), stop=(k == KC - 1))
                    nc.scalar.activation(qkv1[:, m, :], ps[:], AF.Copy)
                # V token-major (+ones col), from h directly (folded weights)
                vloc = vp.tile([128, KC, 8, 65], F16, tag="vloc")
                for t in range(KC):
                    nc.vector.tensor_copy(vloc[:, t, :, 64], ones8[:])
                for t in range(KC):
                    ps = b1.tile([128, S], F32, tag="b1")
                    for k in range(KC):
                        nc.tensor.matmul(
                            ps[:], h[:, k, t * 128:(t + 1) * 128],
                            wv[:, k, :], start=(k == 0), stop=(k == KC - 1))
                    nc.vector.tensor_tensor(
                        vloc[:, t, :, 0:64],
                        ps[:].rearrange("p (h d) -> p h d", h=H),
                        vb_bc[:].rearrange("p (h d) -> p h d", h=H), ALU.add)
                vlocs[s] = vloc
                # rope: qkv1 (de-interleaved) -> qkr (natural order), DVE fp16
                qkr = a8.tile([128, 8, S], F16, tag="a8")
                for half in (0, 4):
                    for c in range(2):
                        e = qkv1[:, half + c, :]
                        o = qkv1[:, half + 2 + c, :]
                        r1 = qkr[:, half + c, :]
                        r2 = qkr[:, half + 2 + c, :]
                        t1 = rtp.tile([128, S], F16, tag="rt")
                        nc.vector.tensor_tensor(r1, e, gridc[:, c, :], ALU.mult)
                        nc.vector.tensor_tensor(t1[:], o, grids[:, c, :], ALU.mult)
                        nc.vector.tensor_tensor(r1, r1, t1[:], ALU.subtract)
                        t2 = rtp.tile([128, S], F16, tag="rt")
                        nc.vector.tensor_tensor(r2, e, grids[:, c, :], ALU.mult)
                        nc.vector.tensor_tensor(t2[:], o, gridc[:, c, :], ALU.mult)
                        nc.vector.tensor_tensor(r2, r2, t2[:], ALU.add)
                qkrs[s] = qkr

            # ---- phase A2 per sample: stage2, attention, out-proj ----
            rc2 = {}
            for s in range(BL):
                sl = slice(s * S, (s + 1) * S)
                qkr, vloc = qkrs[s], vlocs[s]
                qk2 = qk2p.tile([128, 8, S], F16, tag="qk2")
                for m in range(8):
                    ps = b1.tile([128, S], F32, tag="b1")
                    base = 0 if m < 4 else 4
                    for k in range(KC):
                        nc.tensor.matmul(ps[:], w2[:, k, m * 128:(m + 1) * 128],
                                         qkr[:, base + k, :],
                                         start=(k == 0), stop=(k == KC - 1))
                    nc.vector.tensor_tensor(qk2[:, m, :], ps[:], cq[:, m, :],
                                            ALU.add)
                # attention heads (paired: exp(h+1) hides under A@V(h))
                o_t = a4.tile([128, KC, S], F16, tag="ot", bufs=2)
                for h0 in range(0, H, 2):
                    ats = {}
                    for hh in (h0, h0 + 1):
                        rows = slice(64 * (hh % 2), 64 * (hh % 2) + 64)
                        at = a4.tile([128, KC, S], F16, tag="at", bufs=3)
                        for c in range(KC):
                            scp = b1.tile([128, S], F32, tag="b1")
                            nc.tensor.matmul(
                                scp[:],
                                qk2[rows, 4 + hh // 2, c * 128:(c + 1) * 128],
                                qk2[rows, hh // 2, :], start=True, stop=True)
                            nc.scalar.activation(at[:, c, :], scp[:], AF.Exp,
                                                 scale=float(1.0 / np.sqrt(DH)))
                        ats[hh] = at
                    for hh in (h0, h0 + 1):
                        rows = slice(64 * (hh % 2), 64 * (hh % 2) + 64)
                        at = ats[hh]
                        ov = b2.tile([65, S], F32, tag="b2")
                        for c in range(KC):
                            nc.tensor.matmul(ov[:], vloc[:, c, hh, :], at[:, c, :],
                                             start=(c == 0), stop=(c == KC - 1))
                        # 1/denom = exp(-ln denom), broadcast via PE
                        lnd = scr.tile([1, S], F16, tag="lnd")
                        nc.scalar.activation(lnd[:], ov[64:65, :], AF.Ln)
                        rb = b2.tile([64, S], F32, tag="b2")
                        nc.tensor.matmul(rb[:], ones_row[:, 0:64], lnd[:],
                                         start=True, stop=True)
                        inv = invp.tile([64, S], F16, tag="inv")
                        nc.scalar.activation(inv[:], rb[:], AF.Exp, scale=-1.0)
                        nc.vector.tensor_tensor(o_t[rows, hh // 2, :],
                                                ov[0:64, :], inv[:], ALU.mult)
                # out-proj + residual
                for m in range(KC):
                    ps = b1.tile([128, S], F32, tag="b1")
                    for k in range(KC):
                        nc.tensor.matmul(ps[:], ow[:, k, m * 128:(m + 1) * 128],
                                         o_t[:, k, :], start=(k == 0),
                                         stop=(k == KC - 1))
                    nc.vector.scalar_tensor_tensor(
                        x[:, m, sl], ps[:], owb[:, m:m + 1], x[:, m, sl],
                        ALU.add, ALU.add)
                rc2[s] = ln_stats(x, s)

            # ---- phase B: FFN in fp8/DoubleRow (l2 streams in once w1 slot
            #      is released); biases ride in the PSUM via a rank-1 matmul.
            l2 = wpool.tile([128, FC, D], F8, tag="bigw")
            for kf in range(FC):
                nc.sync.dma_start(l2[:, kf, :], dram["el2T"][li][:, kf, :])
            for s in range(BL):
                sl = slice(s * S, (s + 1) * S)
                h2 = hp.tile([128, KC, S], F8, tag="h8")
                ln_apply(rc2[s], x, s, h2, scaled=True)   # h2 = HS*LN(x)
                f2 = [b1.tile([128, S], F32, tag="b1", name=f"f2_{_m}")
                      for _m in range(KC)]
                for m in range(KC):     # inject WOS*l2b into the accumulators
                    nc.tensor.matmul(f2[m][:], l2b[:, m * 128:(m + 1) * 128],
                                     ones_rowS[:], start=True, stop=False)
                for jf in range(FC // 2):
                    rt2 = rtp.tile([128, 2, S], F8, tag="rt8")
                    for i in range(2):
                        kf = 2 * jf + i
                        f1 = b2.tile([128, S], F32, tag="b2")
                        for c in range(2):
                            nc.tensor.matmul(
                                f1[:],
                                l1[:, 2 * c:2 * c + 2, kf * 128:(kf + 1) * 128],
                                h2[:, 2 * c:2 * c + 2, :],
                                start=(c == 0), stop=(c == 1), perf_mode=DR)
                        # rt = OS*relu(z + l1b):  f1 = WHS*z, bias = OS*l1b
                        nc.scalar.activation(rt2[:, i, :], f1[:], AF.Relu,
                                             scale=OS / WHS,
                                             bias=l1b[:, kf:kf + 1])
                    for m in range(KC):
                        nc.tensor.matmul(f2[m][:],
                                         l2[:, 2 * jf:2 * jf + 2, m * 128:(m + 1) * 128],
                                         rt2[:, :, :], start=False,
                                         stop=(jf == FC // 2 - 1), perf_mode=DR)
                for m in range(KC):
                    nc.vector.scalar_tensor_tensor(
                        x[:, m, sl], f2[m][:], 1.0 / WOS, x[:, m, sl],
                        ALU.mult, ALU.add)
                rc1[s] = ln_stats(x, s)   # next layer (or the final LN)

        # ---------------- final encoder LN (in-place; affine folded) --------
        me = x
        for s in range(BL):
            ln_apply(rc1[s], x, s, x[:, :, s * S:(s + 1) * S])

        # ---------------- decoder layers ----------------
        for li in range(ld):
            dw = wpool.tile([128, KC, 3 * D], F16, tag="bigw")
            nc.sync.dma_start(dw[:], dram["dinT"][li])
            dwb = bpool.tile([128, 12], F32, tag="w1b")
            nc.sync.dma_start(dwb[:], dram["dinb"][li])
            dvbrow = bpool.tile([1, D], F16, tag="vbrow")
            nc.sync.dma_start(dvbrow[:], dram["dvb"][li])
            do = owpool.tile([128, KC, D], F16, tag="ow")
            nc.sync.dma_start(do[:], dram["dowT"][li])
            dob = bpool.tile([128, 4], F32, tag="owb")
            nc.sync.dma_start(dob[:], dram["dowb"][li])
            m1 = wpool.tile([128, KC, M], F16, tag="bigw")
            nc.sync.dma_start(m1[:], dram["dm1T"][li])
            m1b = bpool.tile([128, 16], F32, tag="l1b")
            nc.sync.dma_start(m1b[:], dram["dm1b"][li])
            m2b = bpool.tile([128, 4], F32, tag="l2b")
            nc.sync.dma_start(m2b[:], dram["dm2b"][li])

            vb_ps = b1.tile([128, D], F32, tag="b1")
            nc.tensor.matmul(vb_ps[:], ones_row[:], dvbrow[:], start=True, stop=True)
            vb_bc = vbp.tile([128, D], F16, tag="vb_bc")
            nc.scalar.activation(vb_bc[:], vb_ps[:], AF.Copy)

            # LN(p) -> q_ln ; Q projection (all samples at once, N=BL)
            q_ln = smalls.tile([128, KC, BL], F16, tag="q_ln")
            ln_small(p, BL, q_ln)
            qps = b1.tile([128, KC, BL], F32, tag="b1")
            for m in range(KC):
                for k in range(KC):
                    nc.tensor.matmul(qps[:, m, :],
                                     dw[:, k, m * 128:(m + 1) * 128],
                                     q_ln[:, k, :], start=(k == 0),
                                     stop=(k == KC - 1))
            q_sb = smalls.tile([128, KC, BL], F16, tag="q_sb")
            for m in range(KC):
                nc.scalar.activation(q_sb[:, m, :], qps[:, m, :], AF.Identity,
                                     bias=dwb[:, m:m + 1])
            o_d = smalls.tile([128, KC, BL], F16, tag="o_d")
            for s in range(BL):
                sl = slice(s * S, (s + 1) * S)
                # K (feature-major) and V' (token-major) over morph_enc
                k_sb = a4.tile([128, KC, S], F16, tag="at", bufs=3)
                for m in range(KC):
                    ps = b1.tile([128, S], F32, tag="b1")
                    for k in range(KC):
                        nc.tensor.matmul(
                            ps[:], dw[:, k, D + m * 128:D + (m + 1) * 128],
                            me[:, k, sl], start=(k == 0), stop=(k == KC - 1))
                    nc.scalar.activation(k_sb[:, m, :], ps[:], AF.Identity,
                                         bias=dwb[:, 4 + m:5 + m])
                vloc = vp.tile([128, KC, 8, 65], F16, tag="vloc")
                for t in range(KC):
                    nc.vector.tensor_copy(vloc[:, t, :, 64], ones8[:])
                for t in range(KC):
                    ps = b1.tile([128, S], F32, tag="b1")
                    for k in range(KC):
                        nc.tensor.matmul(
                            ps[:], me[:, k, s * S + t * 128:s * S + (t + 1) * 128],
                            dw[:, k, 2 * D:3 * D],
                            start=(k == 0), stop=(k == KC - 1))
                    nc.vector.tensor_tensor(
                        vloc[:, t, :, 0:64],
                        ps[:].rearrange("p (h d) -> p h d", h=H),
                        vb_bc[:].rearrange("p (h d) -> p h d", h=H), ALU.add)
                scp = b1.tile([128, KC, H], F32, tag="b1")
                for hh in range(H):
                    rows = slice(64 * (hh % 2), 64 * (hh % 2) + 64)
                    for c in range(KC):
                        nc.tensor.matmul(
                            scp[:, c, hh:hh + 1],
                            k_sb[rows, hh // 2, c * 128:(c + 1) * 128],
                            q_sb[rows, hh // 2, s:s + 1],
                            start=True, stop=True)
                at = smalls.tile([128, KC, H], F16, tag="at_d")
                nc.scalar.activation(at[:], scp[:], AF.Exp,
                                     scale=float(1.0 / np.sqrt(DH)))
                ov = b2.tile([65, H], F32, tag="b2")
                for hh in range(H):
                    for c in range(KC):
                        nc.tensor.matmul(ov[:, hh:hh + 1], vloc[:, c, hh, :],
                                         at[:, c, hh:hh + 1],
                                         start=(c == 0), stop=(c == KC - 1))
                # 1/denom = exp(-ln denom)
                lnd = scr.tile([1, H], F16, tag="lnd")
                nc.scalar.activation(lnd[:], ov[64:65, :], AF.Ln)
                rb = b2.tile([64, H], F32, tag="b2")
                nc.tensor.matmul(rb[:], ones_row[:, 0:64], lnd[:],
                                 start=True, stop=True)
                inv = invp.tile([64, H], F16, tag="inv_d")
                nc.scalar.activation(inv[:], rb[:], AF.Exp, scale=-1.0)
                for hh in range(H):
                    rows = slice(64 * (hh % 2), 64 * (hh % 2) + 64)
                    nc.vector.tensor_tensor(o_d[rows, hh // 2, s:s + 1],
                                            ov[0:64, hh:hh + 1],
                                            inv[:, hh:hh + 1], ALU.mult)
            # out-proj + residual into p
            ops = b1.tile([128, KC, BL], F32, tag="b1")
            for m in range(KC):
                for k in range(KC):
                    nc.tensor.matmul(ops[:, m, :],
                                     do[:, k, m * 128:(m + 1) * 128],
                                     o_d[:, k, :], start=(k == 0),
                                     stop=(k == KC - 1))
            for m in range(KC):
                nc.vector.scalar_tensor_tensor(
                    p[:, m, :], ops[:, m, :], dob[:, m:m + 1], p[:, m, :],
                    ALU.add, ALU.add)
            # FFN on p (m2 streams in chunked once dw releases its slot)
            m2 = wpool.tile([128, MC, D], F16, tag="bigw")
            for kf in range(MC):
                nc.sync.dma_start(m2[:, kf, :], dram["dm2T"][li][:, kf, :])
            h2d = smalls.tile([128, KC, BL], F16, tag="q_ln")
            ln_small(p, BL, h2d)
            mh = smalls.tile([128, MC, BL], F16, tag="mh")
            for mm_ in range(MC):
                ps = b1.tile([128, BL], F32, tag="b1")
                for k in range(KC):
                    nc.tensor.matmul(ps[:], m1[:, k, mm_ * 128:(mm_ + 1) * 128],
                                     h2d[:, k, :], start=(k == 0),
                                     stop=(k == KC - 1))
                nc.scalar.activation(mh[:, mm_, :], ps[:], AF.Relu,
                                     bias=m1b[:, mm_:mm_ + 1])
            m2ps = b1.tile([128, KC, BL], F32, tag="b1")
            for m in range(KC):
                for kf in range(MC):
                    nc.tensor.matmul(m2ps[:, m, :],
                                     m2[:, kf, m * 128:(m + 1) * 128],
                                     mh[:, kf, :], start=(kf == 0),
                                     stop=(kf == MC - 1))
            for m in range(KC):
                nc.vector.scalar_tensor_tensor(
                    p[:, m, :], m2ps[:, m, :], m2b[:, m:m + 1], p[:, m, :],
                    ALU.add, ALU.add)

        # ---------------- head ----------------
        hw = smalls.tile([128, KC], F16, tag="hw")
        hb = smalls.tile([1, 1], F32, tag="hb")
        nc.sync.dma_start(hw[:], dram["hwT"][:])
        nc.sync.dma_start(hb[:], dram["hb"][:])
        hg = smalls.tile([128, KC, BL], F16, tag="q_ln")
        ln_small(p, BL, hg)
        hps = b2.tile([1, BL], F32, tag="b2")
        for k in range(KC):
            nc.tensor.matmul(hps[:], hw[:, k:k + 1], hg[:, k, :],
                             start=(k == 0), stop=(k == KC - 1))
        y_sb = smalls.tile([1, BL], F32, tag="y_sb")
        nc.scalar.activation(y_sb[:], hps[:], AF.Sigmoid, bias=hb[:])
        nc.sync.dma_start(y_dram[:], y_sb[:])


# ----------------------------------------------------------------------------
# entry point
# ----------------------------------------------------------------------------

_NC_CACHE = {}


def kernel(**inputs):
    return _run(inputs, LE, LD)


def _run(inputs, le, ld, trace=False):
    w = prep_weights(inputs, le, ld)
    morph = np.asarray(inputs["morph"], np.float32)
    pose = np.asarray(inputs["pose"], np.float32)
    in_maps = []
    for c in range(NCORES):
        im = dict(w)
        mo = morph[c * BL:(c + 1) * BL]                 # [BL, S, 3]
        im["morphT"] = np.ascontiguousarray(
            mo.transpose(2, 0, 1).reshape(3, T)).astype(np.float16)
        im["poseT"] = np.ascontiguousarray(
            pose[c * BL:(c + 1) * BL].T).astype(np.float16)
        in_maps.append(im)

    if ("nc", le, ld) not in _NC_CACHE:
        _NC_CACHE[("nc", le, ld)] = build(le, ld)
    nc = _NC_CACHE[("nc", le, ld)]
    res = run_bass_kernel_spmd(nc, in_maps, core_ids=list(range(NCORES)),
                               trace=trace)
    out = np.zeros((B, 1), np.float32)
    for c in range(NCORES):
        out[c * BL:(c + 1) * BL, 0] = res.results[c]["y"][0]
    if trace:
        return out, res
    return out


# revision 55
# speedup vs baseline: 1.8731x; 1.0196x over previous
"""Trainium2 Bass kernel for nn_ReachabilityClassifierTransformer.

Data-parallel over batch: 16 samples / 8 cores = 2 samples per core.
Each core runs the full network (6-layer encoder + 4-layer decoder + head)
on its 2 samples. No collectives.

v2 (fp16 pipeline):
  - All matmul operands fp16 (stationary weights get Fast-Weight-Load; DVE
    elementwise ops run in 2x mode; DMA traffic halved). PSUM stays fp32.
  - V path folded on host: V = h @ (Wv2 Wv1)^T + bv  (no rope between the
    two V projections, so the double-projection quirk collapses).
  - Stage-1 q/k biases folded through rope into per-position bias tensors
    C_q/C_k = W2 @ rope(b1) + b2, added at the stage-2 PSUM copy (rope is
    linear, rotation depends only on position).
  - No Sqrt / no DVE reciprocal anywhere: LN rsqrt = exp(-0.5 ln(v+eps)),
    softmax 1/denom = exp(-ln denom) broadcast via PE.  ln/exp/copy/relu/
    square all live in one activation table set -> no table switches.
  - Per-layer phase order interleaves the two samples so rope (DVE) and
    softmax exp (Act) hide under the other sample's matmuls.

Device layout conventions (per core):
  - Activations FEATURE-MAJOR in SBUF: tile [128, KC, T] holds X.T.
  - Weights pre-transposed on host to [in_feat, out_feat], laid out
    [128, KC_in, O] (partition = in-feature % 128).
  - matmul(out_psum[M,N], lhsT=[K,M], rhs=[K,N]) computes lhsT.T @ rhs.
  - Encoder stage-1 q,k output features are de-interleaved (even feats then
    odd feats) via host-side column permutation, so RoPE becomes contiguous
    block ops; the roped result is in natural (concatenated) order.
  - Softmax: scores computed transposed (S.T = K_h @ Q_h.T per chunk),
    exp'd without max subtraction (|scores/8| < 1 for this model), and the
    denominator comes free from a ones-column appended to V.
"""
import functools

import numpy as np

import concourse.bass as bass
import concourse.mybir as mybir
import concourse.tile as tile
from concourse import bacc
from concourse.bass_utils import run_bass_kernel_spmd


def _patch_act_tables():
    """Constrain exp/ln to the one table set that contains both.

    The act-table-load pass maps each activation function to a set
    independently (exp -> exp_and_others, ln -> natural_log), so a kernel
    that interleaves exp and ln reloads tables on every transition
    (~1.3us each).  natural_log_exp_and_others contains exp AND ln (plus
    copy/identity/relu/square), so restricting exp/ln to that set makes
    every load resolve there; set ids/order are preserved so the emitted
    act_func_set_id still indexes the real act_info.json.
    """
    import concourse.hw_specs as hw_specs
    if getattr(hw_specs, "_ant_act_tables_patched", False):
        return
    orig = hw_specs.get_activation_tables

    @functools.cache
    def patched(module_arch):
        t = orig(module_arch)
        keep = "natural_log_exp_and_others"
        if keep not in t:
            return t
        drop = {mybir.ActivationFunctionType.Exp, mybir.ActivationFunctionType.Ln}
        return {name: (fns if name == keep else fns - drop)
                for name, fns in t.items()}

    hw_specs._ant_act_tables_patched = True
    hw_specs.get_activation_tables = patched
    import sys
    for modname in ("concourse.bacc", "concourse.bass_interp"):
        mod = sys.modules.get(modname)
        if mod is not None and hasattr(mod, "get_activation_tables"):
            mod.get_activation_tables = patched


_patch_act_tables()

AF = mybir.ActivationFunctionType
ALU = mybir.AluOpType
F32 = mybir.dt.float32
F16 = mybir.dt.float16
F8 = mybir.dt.float8e4
F8NP = mybir.dt.np(F8)
DR = mybir.MatmulPerfMode.DoubleRow
F8MAX = 240.0          # TRN fp8e4 saturation (not OCP's 448)
WS = 64.0              # fp8 FFN weight scale
HS = 16.0              # fp8 FFN input activation scale
OS = 32.0              # fp8 FFN hidden scale
WHS = WS * HS          # l1 psum descale
WOS = WS * OS          # l2 psum descale

B, S, D, FF, H, LE, LD, M = 16, 512, 512, 2048, 8, 6, 4, 2048
ROPE_BASE = 10000.0
LN_EPS = 1e-5
NCORES = 8
BL = B // NCORES          # 2 samples per core
T = BL * S                # 1024 tokens per core
KC = D // 128             # 4 feature chunks
FC = FF // 128            # 16
MC = M // 128             # 16
DH = D // H               # 64


# ----------------------------------------------------------------------------
# host-side helpers
# ----------------------------------------------------------------------------

def _chunked(wT):
    """[Din, O] -> [128, Din//128, O] contiguous fp16."""
    Din, O = wT.shape
    return np.ascontiguousarray(
        wT.reshape(Din // 128, 128, O).transpose(1, 0, 2)).astype(np.float16)


def _bias_cols(b):
    """[O] -> [128, O//128]  (column per 128-chunk), fp32."""
    O = b.shape[0]
    return np.ascontiguousarray(b.reshape(O // 128, 128).T).astype(np.float32)


def _chunked8(wT, scale=WS):
    """[Din, O] -> [128, Din//128, O] contiguous fp8e4, pre-scaled."""
    Din, O = wT.shape
    a = np.clip(wT * scale, -F8MAX, F8MAX)
    return np.ascontiguousarray(
        a.reshape(Din // 128, 128, O).transpose(1, 0, 2)).astype(F8NP)


_DEINT = np.concatenate([np.arange(0, D, 2), np.arange(1, D, 2)])  # de-interleave


def prep_weights(inp, le=LE, ld=LD):
    """Host-side weight prep -> dict of arrays shared by all cores."""
    out = {}
    g = {k: np.asarray(v, np.float64) for k, v in inp.items()}

    out["mpwT"] = np.ascontiguousarray(g["morph_proj_w"].T).astype(np.float16)
    out["mpb"] = _bias_cols(g["morph_proj_b"])                     # [128, 4]
    out["ppwT"] = np.ascontiguousarray(g["pose_proj_w"].T).astype(np.float16)
    out["ppb"] = _bias_cols(g["pose_proj_b"])

    # rope grids, de-interleaved frequency order: [128, 2, 512] fp16
    freq = 1.0 / ROPE_BASE ** (np.arange(0, D, 2, dtype=np.float64) / D)
    ang = np.outer(np.arange(S, dtype=np.float64), freq)           # [512, 256]
    cosT = np.cos(ang).T                                           # [256, S]
    sinT = np.sin(ang).T
    out["gridc"] = _chunked(cosT.reshape(256, S))
    out["grids"] = _chunked(sinT.reshape(256, S))

    e_w1, e_w2, e_wv, e_cqk, e_vb = [], [], [], [], []
    e_ow, e_owb, e_l1, e_l1b, e_l2, e_l2b = [], [], [], [], [], []
    for i in range(le):
        w1 = g["enc_in_w"][i] * g["enc_n1_g"][i][None, :]          # fold n1 g
        b1 = g["enc_in_b"][i] + g["enc_in_w"][i] @ g["enc_n1_b"][i]
        # stage-1 q,k only, de-interleaved output columns
        perm = np.concatenate([_DEINT, D + _DEINT])
        e_w1.append(_chunked(np.ascontiguousarray(w1[perm].T)))    # [128,4,1024]
        # stage-2 q,k (natural order, raw weights - the faithful quirk)
        w2 = g["enc_in_w"][i][: 2 * D]                             # Wq;Wk
        e_w2.append(_chunked(np.ascontiguousarray(w2.T)))          # [128,4,1024]
        # stage-2 bias tensors: C = W2 @ rope(b1) + b2   [512, S] each
        bq = b1[:D][_DEINT]                                        # [even; odd]
        bk = b1[D:2 * D][_DEINT]
        rb_q = np.concatenate([bq[:256, None] * cosT - bq[256:, None] * sinT,
                               bq[:256, None] * sinT + bq[256:, None] * cosT])
        rb_k = np.concatenate([bk[:256, None] * cosT - bk[256:, None] * sinT,
                               bk[:256, None] * sinT + bk[256:, None] * cosT])
        Cq = g["enc_in_w"][i][:D] @ rb_q + g["enc_in_b"][i][:D][:, None]
        Ck = g["enc_in_w"][i][D:2 * D] @ rb_k \
            + g["enc_in_b"][i][D:2 * D][:, None]
        C = np.concatenate([Cq, Ck], axis=0)                       # [1024, S]
        e_cqk.append(_chunked(C))                                  # [128,8,S]
        # V folded: V = h @ (Wv2 Wv1_f).T + (Wv2 bv1_f + bv2)
        Wv1f = w1[2 * D:]
        bv1f = b1[2 * D:]
        Wv2 = g["enc_in_w"][i][2 * D:]
        bv2 = g["enc_in_b"][i][2 * D:]
        e_wv.append(_chunked(np.ascontiguousarray((Wv2 @ Wv1f).T)))
        e_vb.append((Wv2 @ bv1f + bv2)[None, :].astype(np.float16))  # [1,512]
        e_ow.append(_chunked(np.ascontiguousarray(g["enc_out_w"][i].T)))
        e_owb.append(_bias_cols(g["enc_out_b"][i]))
        l1 = g["enc_l1_w"][i] * g["enc_n2_g"][i][None, :]
        l1b = g["enc_l1_b"][i] + g["enc_l1_w"][i] @ g["enc_n2_b"][i]
        e_l1.append(_chunked8(np.ascontiguousarray(l1.T)))         # [128,4,2048]
        e_l1b.append(_bias_cols(OS * l1b))                         # [128,16]
        e_l2.append(_chunked8(np.ascontiguousarray(g["enc_l2_w"][i].T)))
        e_l2b.append((WOS * g["enc_l2_b"][i])[None, :].astype(np.float16))
    out["ew1T"] = np.stack(e_w1) if le else np.zeros((0, 128, KC, 2 * D), np.float16)
    out["ew2T"] = np.stack(e_w2) if le else np.zeros((0, 128, KC, 2 * D), np.float16)
    out["ewvT"] = np.stack(e_wv) if le else np.zeros((0, 128, KC, D), np.float16)
    out["ecqk"] = np.stack(e_cqk) if le else np.zeros((0, 128, 8, S), np.float16)
    out["evb"] = np.stack(e_vb) if le else np.zeros((0, 1, D), np.float16)
    out["eowT"] = np.stack(e_ow) if le else np.zeros((0, 128, KC, D), np.float16)
    out["eowb"] = np.stack(e_owb) if le else np.zeros((0, 128, 4), np.float32)
    out["el1T"] = np.stack(e_l1) if le else np.zeros((0, 128, KC, FF), F8NP)
    out["el1b"] = np.stack(e_l1b) if le else np.zeros((0, 128, 16), np.float32)
    out["el2T"] = np.stack(e_l2) if le else np.zeros((0, 128, FC, D), F8NP)
    out["el2b"] = np.stack(e_l2b) if le else np.zeros((0, 1, D), np.float16)

    d_in, d_inb, d_vb, d_ow, d_owb = [], [], [], [], []
    d_m1, d_m1b, d_m2, d_m2b = [], [], [], []
    for i in range(ld):
        w = g["dec_in_w"][i].copy()
        b = g["dec_in_b"][i].copy()
        w[:D] = w[:D] * g["dec_n1_g"][i][None, :]                  # Wq <- dec_n1
        b[:D] = b[:D] + g["dec_in_w"][i][:D] @ g["dec_n1_b"][i]
        w[D:] = w[D:] * g["enc_final_g"][None, :]                  # Wk,Wv <- enc_final
        b[D:] = b[D:] + g["dec_in_w"][i][D:] @ g["enc_final_b"]
        d_in.append(_chunked(np.ascontiguousarray(w.T)))           # [128,4,1536]
        d_inb.append(_bias_cols(b))
        d_vb.append(b[2 * D:][None, :].astype(np.float16))         # [1,512]
        d_ow.append(_chunked(np.ascontiguousarray(g["dec_out_w"][i].T)))
        d_owb.append(_bias_cols(g["dec_out_b"][i]))
        m1 = g["dec_m1_w"][i] * g["dec_n2_g"][i][None, :]
        m1b = g["dec_m1_b"][i] + g["dec_m1_w"][i] @ g["dec_n2_b"][i]
        d_m1.append(_chunked(np.ascontiguousarray(m1.T)))          # [128,4,2048]
        d_m1b.append(_bias_cols(m1b))
        d_m2.append(_chunked(np.ascontiguousarray(g["dec_m2_w"][i].T)))
        d_m2b.append(_bias_cols(g["dec_m2_b"][i]))
    out["dinT"] = np.stack(d_in) if ld else np.zeros((0, 128, KC, 3 * D), np.float16)
    out["dinb"] = np.stack(d_inb) if ld else np.zeros((0, 128, 12), np.float32)
    out["dvb"] = np.stack(d_vb) if ld else np.zeros((0, 1, D), np.float16)
    out["dowT"] = np.stack(d_ow) if ld else np.zeros((0, 128, KC, D), np.float16)
    out["dowb"] = np.stack(d_owb) if ld else np.zeros((0, 128, 4), np.float32)
    out["dm1T"] = np.stack(d_m1) if ld else np.zeros((0, 128, KC, M), np.float16)
    out["dm1b"] = np.stack(d_m1b) if ld else np.zeros((0, 128, 16), np.float32)
    out["dm2T"] = np.stack(d_m2) if ld else np.zeros((0, 128, MC, D), np.float16)
    out["dm2b"] = np.stack(d_m2b) if ld else np.zeros((0, 128, 4), np.float32)

    hw = (g["head_w"] * g["head_g"][None, :])[0]                   # [512]
    out["hwT"] = _bias_cols(hw).astype(np.float16)                 # [128, 4]
    out["hb"] = (g["head_bias"] + g["head_w"] @ g["head_b"]).reshape(1, 1).astype(np.float32)
    return out


# ----------------------------------------------------------------------------
# device program
# ----------------------------------------------------------------------------

def build(le=LE, ld=LD):
    nc = bacc.Bacc(None, target_bir_lowering=False)

    dram = {}

    def din(name, shape, dt=F16):
        dram[name] = nc.dram_tensor(name, list(shape), dt, kind="ExternalInput")
        return dram[name]

    # shared weights
    din("mpwT", [3, 512]); din("mpb", [128, 4], F32)
    din("ppwT", [9, 512]); din("ppb", [128, 4], F32)
    din("gridc", [128, 2, S]); din("grids", [128, 2, S])
    din("ew1T", [le, 128, KC, 2 * D])
    din("ew2T", [le, 128, KC, 2 * D])
    din("ewvT", [le, 128, KC, D])
    din("ecqk", [le, 128, 8, S])
    din("evb", [le, 1, D])
    din("eowT", [le, 128, KC, D]); din("eowb", [le, 128, 4], F32)
    din("el1T", [le, 128, KC, FF], F8); din("el1b", [le, 128, 16], F32)
    din("el2T", [le, 128, FC, D], F8); din("el2b", [le, 1, D])
    din("dinT", [ld, 128, KC, 3 * D]); din("dinb", [ld, 128, 12], F32)
    din("dvb", [ld, 1, D])
    din("dowT", [ld, 128, KC, D]); din("dowb", [ld, 128, 4], F32)
    din("dm1T", [ld, 128, KC, M]); din("dm1b", [ld, 128, 16], F32)
    din("dm2T", [ld, 128, MC, D]); din("dm2b", [ld, 128, 4], F32)
    din("hwT", [128, KC]); din("hb", [1, 1], F32)
    # per-core inputs
    din("morphT", [3, T])
    din("poseT", [9, BL])
    y = nc.dram_tensor("y", [1, BL], F32, kind="ExternalOutput")

    with tile.TileContext(nc) as tc:
        _build_body(nc, tc, dram, y, le, ld)
    nc.compile()
    return nc


def _build_body(nc, tc, dram, y_dram, le, ld):
    import contextlib
    ctx = contextlib.ExitStack()
    with ctx:
        ctx.enter_context(nc.allow_low_precision(
            reason="fp16 matmul operands / activations are intentional"))
        persist = ctx.enter_context(tc.tile_pool(name="persist", bufs=1))
        wpool = ctx.enter_context(tc.tile_pool(name="wpool", bufs=2))
        w2pool = ctx.enter_context(tc.tile_pool(name="w2pool", bufs=1))
        owpool = ctx.enter_context(tc.tile_pool(name="owpool", bufs=1))
        vwpool = ctx.enter_context(tc.tile_pool(name="vwpool", bufs=1))
        cqpool = ctx.enter_context(tc.tile_pool(name="cqpool", bufs=1))
        bpool = ctx.enter_context(tc.tile_pool(name="bpool", bufs=2))
        a4 = ctx.enter_context(tc.tile_pool(name="a4", bufs=4))
        hp = ctx.enter_context(tc.tile_pool(name="hp", bufs=2))
        a8 = ctx.enter_context(tc.tile_pool(name="a8", bufs=3))
        qk2p = ctx.enter_context(tc.tile_pool(name="qk2p", bufs=2))
        vp = ctx.enter_context(tc.tile_pool(name="vp", bufs=2))
        rtp = ctx.enter_context(tc.tile_pool(name="rtp", bufs=3))
        vbp = ctx.enter_context(tc.tile_pool(name="vbp", bufs=1))
        scr = ctx.enter_context(tc.tile_pool(name="scr", bufs=3))
        invp = ctx.enter_context(tc.tile_pool(name="invp", bufs=2))
        rcp = ctx.enter_context(tc.tile_pool(name="rcp", bufs=2))
        smalls = ctx.enter_context(tc.tile_pool(name="smalls", bufs=2))
        b1 = ctx.enter_context(tc.tile_pool(name="b1", bufs=4, space="PSUM"))
        b2 = ctx.enter_context(tc.tile_pool(name="b2", bufs=4, space="PSUM"))

        # ---------------- persistent tiles ----------------
        x = persist.tile([128, KC, T], F16)           # residual stream (X.T)
        gridc = persist.tile([128, 2, S], F16)
        grids = persist.tile([128, 2, S], F16)
        ones128 = persist.tile([128, 1], F16)
        ones_row = persist.tile([1, 128], F16)
        ones_rowS = persist.tile([1, S], F16)         # bias-inject moving row
        hs_row = persist.tile([1, 128], F16)          # HS-scaled broadcast row
        ones8 = persist.tile([128, 8], F16)
        eps_t = persist.tile([1, 1], F32)
        p = persist.tile([128, KC, BL], F16)          # decoder latent p.T
        nc.sync.dma_start(gridc[:], dram["gridc"][:])
        nc.sync.dma_start(grids[:], dram["grids"][:])
        stage_f16 = rtp.tile([128, 128], F16, tag="rt")
        nc.vector.memset(stage_f16[:], 1.0)
        nc.vector.tensor_copy(ones128[:], stage_f16[:, 0:1])
        nc.vector.tensor_copy(ones_row[:], stage_f16[0:1, :])
        nc.vector.tensor_copy(ones8[:], stage_f16[:, 0:8])
        nc.vector.memset(hs_row[:], HS)
        nc.vector.memset(ones_rowS[:], 1.0)
        nc.vector.memset(eps_t[:], LN_EPS)

        def ln_small(x_tile, n_tok, h_out):
            """h_out = LayerNorm_features(x_tile) for tiny n_tok (decoder)."""
            sq = smalls.tile([128, KC, n_tok], F16, tag="sq_d")
            for k in range(KC):
                nc.vector.tensor_tensor(sq[:, k, :], x_tile[:, k, :],
                                        x_tile[:, k, :], ALU.mult)
            sum_ps = b2.tile([1, n_tok], F32, tag="b2")
            sq_ps = b2.tile([1, n_tok], F32, tag="b2")
            for k in range(KC):
                nc.tensor.matmul(sum_ps[:], ones128[:], x_tile[:, k, :],
                                 start=(k == 0), stop=(k == KC - 1))
            for k in range(KC):
                nc.tensor.matmul(sq_ps[:], ones128[:], sq[:, k, :],
                                 start=(k == 0), stop=(k == KC - 1))
            ms = scr.tile([1, n_tok], F32, tag="scr")
            t2 = scr.tile([1, n_tok], F32, tag="scr")
            rc = scr.tile([1, 2, n_tok], F16, tag="scr_rc")
            nc.scalar.activation(ms[:], sum_ps[:], AF.Copy, scale=1.0 / D)
            nc.vector.tensor_tensor(t2[:], ms[:], ms[:], ALU.mult)      # m^2
            nc.vector.scalar_tensor_tensor(
                t2[:], sq_ps[:], 1.0 / D, t2[:], ALU.mult, ALU.subtract)
            # r = exp(-0.5 ln(var + eps))
            nc.scalar.activation(t2[:], t2[:], AF.Ln, bias=eps_t[:])
            nc.scalar.activation(rc[:, 0, :], t2[:], AF.Exp, scale=-0.5)
            nc.vector.scalar_tensor_tensor(
                rc[:, 1, :], ms[:], -1.0, rc[:, 0, :], ALU.mult, ALU.mult)
            r_ps = b1.tile([128, n_tok], F32, tag="b1")
            c_ps = b1.tile([128, n_tok], F32, tag="b1")
            nc.tensor.matmul(r_ps[:], ones_row[:], rc[:, 0, :],
                             start=True, stop=True)
            nc.tensor.matmul(c_ps[:], ones_row[:], rc[:, 1, :],
                             start=True, stop=True)
            for k in range(KC):
                nc.vector.tensor_tensor(h_out[:, k, :], x_tile[:, k, :],
                                        r_ps[:], ALU.mult)
                nc.vector.tensor_tensor(h_out[:, k, :], h_out[:, k, :],
                                        c_ps[:], ALU.add)

        def ln_stats(x_tile, s):
            """Per-sample LN stats -> rc [1, 2, S] fp16 (r, c)."""
            sl = slice(s * S, (s + 1) * S)
            sq = a4.tile([128, KC, S], F16, tag="sq", bufs=2)
            nc.vector.tensor_tensor(sq[:], x_tile[:, :, sl], x_tile[:, :, sl],
                                    ALU.mult)
            sum_ps = b2.tile([1, S], F32, tag="b2")
            sq_ps = b2.tile([1, S], F32, tag="b2")
            for k in range(KC):
                nc.tensor.matmul(sum_ps[:], ones128[:], x_tile[:, k, sl],
                                 start=(k == 0), stop=(k == KC - 1))
            for k in range(KC):
                nc.tensor.matmul(sq_ps[:], ones128[:], sq[:, k, :],
                                 start=(k == 0), stop=(k == KC - 1))
            ms = scr.tile([1, S], F32, tag="scr")
            t2 = scr.tile([1, S], F32, tag="scr")
            rc = rcp.tile([1, 2, S], F16, tag="rc", bufs=4)
            nc.scalar.activation(ms[:], sum_ps[:], AF.Copy, scale=1.0 / D)
            nc.vector.tensor_tensor(t2[:], ms[:], ms[:], ALU.mult)
            nc.vector.scalar_tensor_tensor(
                t2[:], sq_ps[:], 1.0 / D, t2[:], ALU.mult, ALU.subtract)
            nc.scalar.activation(t2[:], t2[:], AF.Ln, bias=eps_t[:])
            nc.scalar.activation(rc[:, 0, :], t2[:], AF.Exp, scale=-0.5)
            nc.vector.scalar_tensor_tensor(
                rc[:, 1, :], ms[:], -1.0, rc[:, 0, :], ALU.mult, ALU.mult)
            return rc

        def ln_apply(rc, x_tile, s, h_out, scaled=False):
            """h_out = (x[:, :, s] * r + c) * (HS if scaled else 1)."""
            sl = slice(s * S, (s + 1) * S)
            row = hs_row if scaled else ones_row
            r_ps = b1.tile([128, S], F32, tag="b1")
            c_ps = b1.tile([128, S], F32, tag="b1")
            nc.tensor.matmul(r_ps[:], row[:], rc[:, 0, :],
                             start=True, stop=True)
            nc.tensor.matmul(c_ps[:], row[:], rc[:, 1, :],
                             start=True, stop=True)
            rcb = rcp.tile([128, 2, S], F16, tag="rcb")
            nc.scalar.activation(rcb[:, 0, :], r_ps[:], AF.Copy)
            nc.scalar.activation(rcb[:, 1, :], c_ps[:], AF.Copy)
            for k in range(KC):
                tmp = rtp.tile([128, S], F16, tag="rt")
                nc.vector.tensor_tensor(tmp[:], x_tile[:, k, sl],
                                        rcb[:, 0, :], ALU.mult)
                nc.vector.tensor_tensor(h_out[:, k, :], tmp[:],
                                        rcb[:, 1, :], ALU.add)

        # ---------------- morph projection -> x ----------------
        morpht = rtp.tile([3, T], F16, tag="morph", bufs=1)
        nc.sync.dma_start(morpht[:], dram["morphT"][:])
        mpw = rtp.tile([3, 512], F16, tag="rt")
        mpb = rtp.tile([128, 4], F32, tag="rtb")
        nc.sync.dma_start(mpw[:], dram["mpwT"][:])
        nc.sync.dma_start(mpb[:], dram["mpb"][:])
        rc1, hs_next = {}, {}
        for s in range(BL):
            for m in range(KC):
                ps = b1.tile([128, S], F32, tag="b1")
                nc.tensor.matmul(ps[:], mpw[:, m * 128:(m + 1) * 128],
                                 morpht[:, s * S:(s + 1) * S], start=True, stop=True)
                nc.scalar.activation(x[:, m, s * S:(s + 1) * S], ps[:], AF.Relu,
                                     bias=mpb[:, m:m + 1])
            rc1[s] = ln_stats(x, s)
            hs_next[s] = hp.tile([128, KC, S], F16, tag="h", name=f"h_pre{s}")
            ln_apply(rc1[s], x, s, hs_next[s])

        # ---------------- pose projection -> p ----------------
        poset = rtp.tile([9, BL], F16, tag="rt")
        ppw = rtp.tile([9, 512], F16, tag="rt")
        ppb = rtp.tile([128, 4], F32, tag="rtb")
        nc.sync.dma_start(poset[:], dram["poseT"][:])
        nc.sync.dma_start(ppw[:], dram["ppwT"][:])
        nc.sync.dma_start(ppb[:], dram["ppb"][:])
        pps = b1.tile([128, KC, BL], F32, tag="b1")
        for m in range(KC):
            nc.tensor.matmul(pps[:, m, :], ppw[:, m * 128:(m + 1) * 128],
                             poset[:], start=True, stop=True)
        for m in range(KC):
            nc.scalar.activation(p[:, m, :], pps[:, m, :], AF.Relu,
                                 bias=ppb[:, m:m + 1])

        # ---------------- encoder layers ----------------
        for li in range(le):
            w1 = wpool.tile([128, KC, 2 * D], F16, tag="bigw")
            nc.sync.dma_start(w1[:], dram["ew1T"][li])
            w2 = w2pool.tile([128, KC, 2 * D], F16, tag="w2")
            nc.sync.dma_start(w2[:], dram["ew2T"][li])
            wv = vwpool.tile([128, KC, D], F16, tag="wv")
            nc.sync.dma_start(wv[:], dram["ewvT"][li])
            cq = cqpool.tile([128, 8, S], F16, tag="cq")
            nc.sync.dma_start(cq[:], dram["ecqk"][li])
            vbrow = bpool.tile([1, D], F16, tag="vbrow")
            nc.sync.dma_start(vbrow[:], dram["evb"][li])
            ow = owpool.tile([128, KC, D], F16, tag="ow")
            nc.sync.dma_start(ow[:], dram["eowT"][li])
            owb = bpool.tile([128, 4], F32, tag="owb")
            nc.sync.dma_start(owb[:], dram["eowb"][li])
            l1 = wpool.tile([128, KC, FF], F8, tag="bigw")
            nc.sync.dma_start(l1[:], dram["el1T"][li])
            l1b = bpool.tile([128, 16], F32, tag="l1b")
            nc.sync.dma_start(l1b[:], dram["el1b"][li])
            l2b = bpool.tile([1, D], F16, tag="l2br")
            nc.sync.dma_start(l2b[:], dram["el2b"][li])

            # v-bias broadcast [128, 512], once per layer
            vb_ps = b1.tile([128, D], F32, tag="b1")
            nc.tensor.matmul(vb_ps[:], ones_row[:], vbrow[:], start=True, stop=True)
            vb_bc = vbp.tile([128, D], F16, tag="vb_bc")
            nc.scalar.activation(vb_bc[:], vb_ps[:], AF.Copy)

            qkrs, vlocs = {}, {}
            # ---- phase A1 per sample: stage1 q,k; V; rope ----
            # (LN1 stats AND apply were emitted at the end of the previous
            #  phase so the whole chain overlapped that phase's matmuls)
            for s in range(BL):
                h = hs_next[s]
                qkv1 = a8.tile([128, 8, S], F16, tag="a8")
                for m in range(8):
                    ps = b1.tile([128, S], F32, tag="b1")
                    for k in range(KC):
                        nc.tensor.matmul(ps[:], w1[:, k, m * 128:(m + 1) * 128],
                                         h[:, k, :], start=(k == 0), stop=(k == KC - 1))
                    nc.scalar.activation(qkv1[:, m, :], ps[:], AF.Copy)
                # V token-major (+ones col), from h directly (folded weights)
                vloc = vp.tile([128, KC, 8, 65], F16, tag="vloc")
                for t in range(KC):
                    nc.vector.tensor_copy(vloc[:, t, :, 64], ones8[:])
                for t in range(KC):
                    ps = b1.tile([128, S], F32, tag="b1")
                    for k in range(KC):
                        nc.tensor.matmul(
                            ps[:], h[:, k, t * 128:(t + 1) * 128],
                            wv[:, k, :], start=(k == 0), stop=(k == KC - 1))
                    nc.vector.tensor_tensor(
                        vloc[:, t, :, 0:64],
                        ps[:].rearrange("p (h d) -> p h d", h=H),
                        vb_bc[:].rearrange("p (h d) -> p h d", h=H), ALU.add)
                vlocs[s] = vloc
                # rope: qkv1 (de-interleaved) -> qkr (natural order), DVE fp16
                qkr = a8.tile([128, 8, S], F16, tag="a8")
                for half in (0, 4):
                    for c in range(2):
                        e = qkv1[:, half + c, :]
                        o = qkv1[:, half + 2 + c, :]
                        r1 = qkr[:, half + c, :]
                        r2 = qkr[:, half + 2 + c, :]
                        t1 = rtp.tile([128, S], F16, tag="rt")
                        nc.vector.tensor_tensor(r1, e, gridc[:, c, :], ALU.mult)
                        nc.vector.tensor_tensor(t1[:], o, grids[:, c, :], ALU.mult)
                        nc.vector.tensor_tensor(r1, r1, t1[:], ALU.subtract)
                        t2 = rtp.tile([128, S], F16, tag="rt")
                        nc.vector.tensor_tensor(r2, e, grids[:, c, :], ALU.mult)
                        nc.vector.tensor_tensor(t2[:], o, gridc[:, c, :], ALU.mult)
                        nc.vector.tensor_tensor(r2, r2, t2[:], ALU.add)
                qkrs[s] = qkr

            # ---- phase A2 per sample: stage2, attention, out-proj ----
            rc2, h2s = {}, {}
            for s in range(BL):
                sl = slice(s * S, (s + 1) * S)
                qkr, vloc = qkrs[s], vlocs[s]
                qk2 = qk2p.tile([128, 8, S], F16, tag="qk2")
                for m in range(8):
                    ps = b1.tile([128, S], F32, tag="b1")
                    base = 0 if m < 4 else 4
                    for k in range(KC):
                        nc.tensor.matmul(ps[:], w2[:, k, m * 128:(m + 1) * 128],
                                         qkr[:, base + k, :],
                                         start=(k == 0), stop=(k == KC - 1))
                    nc.vector.tensor_tensor(qk2[:, m, :], ps[:], cq[:, m, :],
                                            ALU.add)
                # attention heads, software-pipelined one pair ahead:
                # scores+exp of pair k+1 are emitted before A@V of pair k, so
                # the PE fills pair k's exp latency with pair k+1's scores.
                o_t = a4.tile([128, KC, S], F16, tag="ot", bufs=2)
                ats = {}

                def emit_scores(pair):
                    for hh in pair:
                        rows = slice(64 * (hh % 2), 64 * (hh % 2) + 64)
                        at = a4.tile([128, KC, S], F16, tag="at", bufs=4,
                                     name=f"at{hh}")
                        for c in range(KC):
                            scp = b1.tile([128, S], F32, tag="b1")
                            nc.tensor.matmul(
                                scp[:],
                                qk2[rows, 4 + hh // 2, c * 128:(c + 1) * 128],
                                qk2[rows, hh // 2, :], start=True, stop=True)
                            nc.scalar.activation(at[:, c, :], scp[:], AF.Exp,
                                                 scale=float(1.0 / np.sqrt(DH)))
                        ats[hh] = at

                pairs = [(0, 1), (2, 3), (4, 5), (6, 7)]
                emit_scores(pairs[0])
                for pi, pair in enumerate(pairs):
                    if pi + 1 < len(pairs):
                        emit_scores(pairs[pi + 1])
                    for hh in pair:
                        rows = slice(64 * (hh % 2), 64 * (hh % 2) + 64)
                        at = ats[hh]
                        ov = b2.tile([65, S], F32, tag="b2")
                        for c in range(KC):
                            nc.tensor.matmul(ov[:], vloc[:, c, hh, :], at[:, c, :],
                                             start=(c == 0), stop=(c == KC - 1))
                        # 1/denom = exp(-ln denom), broadcast via PE
                        lnd = scr.tile([1, S], F16, tag="lnd")
                        nc.scalar.activation(lnd[:], ov[64:65, :], AF.Ln)
                        rb = b2.tile([64, S], F32, tag="b2")
                        nc.tensor.matmul(rb[:], ones_row[:, 0:64], lnd[:],
                                         start=True, stop=True)
                        inv = invp.tile([64, S], F16, tag="inv")
                        nc.scalar.activation(inv[:], rb[:], AF.Exp, scale=-1.0)
                        nc.vector.tensor_tensor(o_t[rows, hh // 2, :],
                                                ov[0:64, :], inv[:], ALU.mult)
                # out-proj + residual
                for m in range(KC):
                    ps = b1.tile([128, S], F32, tag="b1")
                    for k in range(KC):
                        nc.tensor.matmul(ps[:], ow[:, k, m * 128:(m + 1) * 128],
                                         o_t[:, k, :], start=(k == 0),
                                         stop=(k == KC - 1))
                    nc.vector.scalar_tensor_tensor(
                        x[:, m, sl], ps[:], owb[:, m:m + 1], x[:, m, sl],
                        ALU.add, ALU.add)
                rc2[s] = ln_stats(x, s)
                h2s[s] = hp.tile([128, KC, S], F8, tag="h8", name=f"h2_{s}")
                ln_apply(rc2[s], x, s, h2s[s], scaled=True)

            # ---- phase B: FFN in fp8/DoubleRow (l2 streams in once w1 slot
            #      is released); biases ride in the PSUM via a rank-1 matmul.
            l2 = wpool.tile([128, FC, D], F8, tag="bigw")
            for kf in range(FC):
                nc.sync.dma_start(l2[:, kf, :], dram["el2T"][li][:, kf, :])
            for s in range(BL):
                sl = slice(s * S, (s + 1) * S)
                h2 = h2s[s]                               # h2 = HS*LN(x)
                f2 = [b1.tile([128, S], F32, tag="b1", name=f"f2_{_m}")
                      for _m in range(KC)]
                for m in range(KC):     # inject WOS*l2b into the accumulators
                    nc.tensor.matmul(f2[m][:], l2b[:, m * 128:(m + 1) * 128],
                                     ones_rowS[:], start=True, stop=False)
                for jf in range(FC // 2):
                    rt2 = rtp.tile([128, 2, S], F8, tag="rt8")
                    for i in range(2):
                        kf = 2 * jf + i
                        f1 = b2.tile([128, S], F32, tag="b2")
                        for c in range(2):
                            nc.tensor.matmul(
                                f1[:],
                                l1[:, 2 * c:2 * c + 2, kf * 128:(kf + 1) * 128],
                                h2[:, 2 * c:2 * c + 2, :],
                                start=(c == 0), stop=(c == 1), perf_mode=DR)
                        # rt = OS*relu(z + l1b):  f1 = WHS*z, bias = OS*l1b
                        nc.scalar.activation(rt2[:, i, :], f1[:], AF.Relu,
                                             scale=OS / WHS,
                                             bias=l1b[:, kf:kf + 1])
                    for m in range(KC):
                        nc.tensor.matmul(f2[m][:],
                                         l2[:, 2 * jf:2 * jf + 2, m * 128:(m + 1) * 128],
                                         rt2[:, :, :], start=False,
                                         stop=(jf == FC // 2 - 1), perf_mode=DR)
                for m in range(KC):
                    nc.vector.scalar_tensor_tensor(
                        x[:, m, sl], f2[m][:], 1.0 / WOS, x[:, m, sl],
                        ALU.mult, ALU.add)
                rc1[s] = ln_stats(x, s)   # next layer (or the final LN)
                if li + 1 < le:           # prefetch next layer's LN1 apply
                    hs_next[s] = hp.tile([128, KC, S], F16, tag="h",
                                         name=f"h_pre{s}")
                    ln_apply(rc1[s], x, s, hs_next[s])

        # ---------------- final encoder LN (in-place; affine folded) --------
        me = x
        for s in range(BL):
            ln_apply(rc1[s], x, s, x[:, :, s * S:(s + 1) * S])

        # ---------------- decoder layers ----------------
        for li in range(ld):
            dw = wpool.tile([128, KC, 3 * D], F16, tag="bigw")
            nc.sync.dma_start(dw[:], dram["dinT"][li])
            dwb = bpool.tile([128, 12], F32, tag="w1b")
            nc.sync.dma_start(dwb[:], dram["dinb"][li])
            dvbrow = bpool.tile([1, D], F16, tag="vbrow")
            nc.sync.dma_start(dvbrow[:], dram["dvb"][li])
            do = owpool.tile([128, KC, D], F16, tag="ow")
            nc.sync.dma_start(do[:], dram["dowT"][li])
            dob = bpool.tile([128, 4], F32, tag="owb")
            nc.sync.dma_start(dob[:], dram["dowb"][li])
            m1 = wpool.tile([128, KC, M], F16, tag="bigw")
            nc.sync.dma_start(m1[:], dram["dm1T"][li])
            m1b = bpool.tile([128, 16], F32, tag="l1b")
            nc.sync.dma_start(m1b[:], dram["dm1b"][li])
            m2b = bpool.tile([128, 4], F32, tag="l2b")
            nc.sync.dma_start(m2b[:], dram["dm2b"][li])

            vb_ps = b1.tile([128, D], F32, tag="b1")
            nc.tensor.matmul(vb_ps[:], ones_row[:], dvbrow[:], start=True, stop=True)
            vb_bc = vbp.tile([128, D], F16, tag="vb_bc")
            nc.scalar.activation(vb_bc[:], vb_ps[:], AF.Copy)

            # LN(p) -> q_ln ; Q projection (all samples at once, N=BL)
            q_ln = smalls.tile([128, KC, BL], F16, tag="q_ln")
            ln_small(p, BL, q_ln)
            qps = b1.tile([128, KC, BL], F32, tag="b1")
            for m in range(KC):
                for k in range(KC):
                    nc.tensor.matmul(qps[:, m, :],
                                     dw[:, k, m * 128:(m + 1) * 128],
                                     q_ln[:, k, :], start=(k == 0),
                                     stop=(k == KC - 1))
            q_sb = smalls.tile([128, KC, BL], F16, tag="q_sb")
            for m in range(KC):
                nc.scalar.activation(q_sb[:, m, :], qps[:, m, :], AF.Identity,
                                     bias=dwb[:, m:m + 1])
            o_d = smalls.tile([128, KC, BL], F16, tag="o_d")
            for s in range(BL):
                sl = slice(s * S, (s + 1) * S)
                # K (feature-major) and V' (token-major) over morph_enc
                k_sb = a4.tile([128, KC, S], F16, tag="at", bufs=4)
                for m in range(KC):
                    ps = b1.tile([128, S], F32, tag="b1")
                    for k in range(KC):
                        nc.tensor.matmul(
                            ps[:], dw[:, k, D + m * 128:D + (m + 1) * 128],
                            me[:, k, sl], start=(k == 0), stop=(k == KC - 1))
                    nc.scalar.activation(k_sb[:, m, :], ps[:], AF.Identity,
                                         bias=dwb[:, 4 + m:5 + m])
                vloc = vp.tile([128, KC, 8, 65], F16, tag="vloc")
                for t in range(KC):
                    nc.vector.tensor_copy(vloc[:, t, :, 64], ones8[:])
                for t in range(KC):
                    ps = b1.tile([128, S], F32, tag="b1")
                    for k in range(KC):
                        nc.tensor.matmul(
                            ps[:], me[:, k, s * S + t * 128:s * S + (t + 1) * 128],
                            dw[:, k, 2 * D:3 * D],
                            start=(k == 0), stop=(k == KC - 1))
                    nc.vector.tensor_tensor(
                        vloc[:, t, :, 0:64],
                        ps[:].rearrange("p (h d) -> p h d", h=H),
                        vb_bc[:].rearrange("p (h d) -> p h d", h=H), ALU.add)
                scp = b1.tile([128, KC, H], F32, tag="b1")
                for hh in range(H):
                    rows = slice(64 * (hh % 2), 64 * (hh % 2) + 64)
                    for c in range(KC):
                        nc.tensor.matmul(
                            scp[:, c, hh:hh + 1],
                            k_sb[rows, hh // 2, c * 128:(c + 1) * 128],
                            q_sb[rows, hh // 2, s:s + 1],
                            start=True, stop=True)
                at = smalls.tile([128, KC, H], F16, tag="at_d")
                nc.scalar.activation(at[:], scp[:], AF.Exp,
                                     scale=float(1.0 / np.sqrt(DH)))
                ov = b2.tile([65, H], F32, tag="b2")
                for hh in range(H):
                    for c in range(KC):
                        nc.tensor.matmul(ov[:, hh:hh + 1], vloc[:, c, hh, :],
                                         at[:, c, hh:hh + 1],
                                         start=(c == 0), stop=(c == KC - 1))
                # 1/denom = exp(-ln denom)
                lnd = scr.tile([1, H], F16, tag="lnd")
                nc.scalar.activation(lnd[:], ov[64:65, :], AF.Ln)
                rb = b2.tile([64, H], F32, tag="b2")
                nc.tensor.matmul(rb[:], ones_row[:, 0:64], lnd[:],
                                 start=True, stop=True)
                inv = invp.tile([64, H], F16, tag="inv_d")
                nc.scalar.activation(inv[:], rb[:], AF.Exp, scale=-1.0)
                for hh in range(H):
                    rows = slice(64 * (hh % 2), 64 * (hh % 2) + 64)
                    nc.vector.tensor_tensor(o_d[rows, hh // 2, s:s + 1],
                                            ov[0:64, hh:hh + 1],
                                            inv[:, hh:hh + 1], ALU.mult)
            # out-proj + residual into p
            ops = b1.tile([128, KC, BL], F32, tag="b1")
            for m in range(KC):
                for k in range(KC):
                    nc.tensor.matmul(ops[:, m, :],
                                     do[:, k, m * 128:(m + 1) * 128],
                                     o_d[:, k, :], start=(k == 0),
                                     stop=(k == KC - 1))
            for m in range(KC):
                nc.vector.scalar_tensor_tensor(
                    p[:, m, :], ops[:, m, :], dob[:, m:m + 1], p[:, m, :],
                    ALU.add, ALU.add)
            # FFN on p (m2 streams in chunked once dw releases its slot)
            m2 = wpool.tile([128, MC, D], F16, tag="bigw")
            for kf in range(MC):
                nc.sync.dma_start(m2[:, kf, :], dram["dm2T"][li][:, kf, :])
            h2d = smalls.tile([128, KC, BL], F16, tag="q_ln")
            ln_small(p, BL, h2d)
            mh = smalls.tile([128, MC, BL], F16, tag="mh")
            for mm_ in range(MC):
                ps = b1.tile([128, BL], F32, tag="b1")
                for k in range(KC):
                    nc.tensor.matmul(ps[:], m1[:, k, mm_ * 128:(mm_ + 1) * 128],
                                     h2d[:, k, :], start=(k == 0),
                                     stop=(k == KC - 1))
                nc.scalar.activation(mh[:, mm_, :], ps[:], AF.Relu,
                                     bias=m1b[:, mm_:mm_ + 1])
            m2ps = b1.tile([128, KC, BL], F32, tag="b1")
            for m in range(KC):
                for kf in range(MC):
                    nc.tensor.matmul(m2ps[:, m, :],
                                     m2[:, kf, m * 128:(m + 1) * 128],
                                     mh[:, kf, :], start=(kf == 0),
                                     stop=(kf == MC - 1))
            for m in range(KC):
                nc.vector.scalar_tensor_tensor(
                    p[:, m, :], m2ps[:, m, :], m2b[:, m:m + 1], p[:, m, :],
                    ALU.add, ALU.add)

        # ---------------- head ----------------
        hw = smalls.tile([128, KC], F16, tag="hw")
        hb = smalls.tile([1, 1], F32, tag="hb")
        nc.sync.dma_start(hw[:], dram["hwT"][:])
        nc.sync.dma_start(hb[:], dram["hb"][:])
        hg = smalls.tile([128, KC, BL], F16, tag="q_ln")
        ln_small(p, BL, hg)
        hps = b2.tile([1, BL], F32, tag="b2")
        for k in range(KC):
            nc.tensor.matmul(hps[:], hw[:, k:k + 1], hg[:, k, :],
                             start=(k == 0), stop=(k == KC - 1))
        y_sb = smalls.tile([1, BL], F32, tag="y_sb")
        nc.scalar.activation(y_sb[:], hps[:], AF.Sigmoid, bias=hb[:])
        nc.sync.dma_start(y_dram[:], y_sb[:])


# ----------------------------------------------------------------------------
# entry point
# ----------------------------------------------------------------------------

_NC_CACHE = {}


def kernel(**inputs):
    return _run(inputs, LE, LD)


def _run(inputs, le, ld, trace=False):
    w = prep_weights(inputs, le, ld)
    morph = np.asarray(inputs["morph"], np.float32)
    pose = np.asarray(inputs["pose"], np.float32)
    in_maps = []
    for c in range(NCORES):
        im = dict(w)
        mo = morph[c * BL:(c + 1) * BL]                 # [BL, S, 3]
        im["morphT"] = np.ascontiguousarray(
            mo.transpose(2, 0, 1).reshape(3, T)).astype(np.float16)
        im["poseT"] = np.ascontiguousarray(
            pose[c * BL:(c + 1) * BL].T).astype(np.float16)
        in_maps.append(im)

    if ("nc", le, ld) not in _NC_CACHE:
        _NC_CACHE[("nc", le, ld)] = build(le, ld)
    nc = _NC_CACHE[("nc", le, ld)]
    res = run_bass_kernel_spmd(nc, in_maps, core_ids=list(range(NCORES)),
                               trace=trace)
    out = np.zeros((B, 1), np.float32)
    for c in range(NCORES):
        out[c * BL:(c + 1) * BL, 0] = res.results[c]["y"][0]
    if trace:
        return out, res
    return out


# revision 59
# speedup vs baseline: 1.8758x; 1.0014x over previous
"""Trainium2 Bass kernel for nn_ReachabilityClassifierTransformer.

Data-parallel over batch: 16 samples / 8 cores = 2 samples per core.
Each core runs the full network (6-layer encoder + 4-layer decoder + head)
on its 2 samples. No collectives.

v2 (fp16 pipeline):
  - All matmul operands fp16 (stationary weights get Fast-Weight-Load; DVE
    elementwise ops run in 2x mode; DMA traffic halved). PSUM stays fp32.
  - V path folded on host: V = h @ (Wv2 Wv1)^T + bv  (no rope between the
    two V projections, so the double-projection quirk collapses).
  - Stage-1 q/k biases folded through rope into per-position bias tensors
    C_q/C_k = W2 @ rope(b1) + b2, added at the stage-2 PSUM copy (rope is
    linear, rotation depends only on position).
  - No Sqrt / no DVE reciprocal anywhere: LN rsqrt = exp(-0.5 ln(v+eps)),
    softmax 1/denom = exp(-ln denom) broadcast via PE.  ln/exp/copy/relu/
    square all live in one activation table set -> no table switches.
  - Per-layer phase order interleaves the two samples so rope (DVE) and
    softmax exp (Act) hide under the other sample's matmuls.

Device layout conventions (per core):
  - Activations FEATURE-MAJOR in SBUF: tile [128, KC, T] holds X.T.
  - Weights pre-transposed on host to [in_feat, out_feat], laid out
    [128, KC_in, O] (partition = in-feature % 128).
  - matmul(out_psum[M,N], lhsT=[K,M], rhs=[K,N]) computes lhsT.T @ rhs.
  - Encoder stage-1 q,k output features are de-interleaved (even feats then
    odd feats) via host-side column permutation, so RoPE becomes contiguous
    block ops; the roped result is in natural (concatenated) order.
  - Softmax: scores computed transposed (S.T = K_h @ Q_h.T per chunk),
    exp'd without max subtraction (|scores/8| < 1 for this model), and the
    denominator comes free from a ones-column appended to V.
"""
import functools

import numpy as np

import concourse.bass as bass
import concourse.mybir as mybir
import concourse.tile as tile
from concourse import bacc
from concourse.bass_utils import run_bass_kernel_spmd


def _patch_act_tables():
    """Constrain exp/ln to the one table set that contains both.

    The act-table-load pass maps each activation function to a set
    independently (exp -> exp_and_others, ln -> natural_log), so a kernel
    that interleaves exp and ln reloads tables on every transition
    (~1.3us each).  natural_log_exp_and_others contains exp AND ln (plus
    copy/identity/relu/square), so restricting exp/ln to that set makes
    every load resolve there; set ids/order are preserved so the emitted
    act_func_set_id still indexes the real act_info.json.
    """
    import concourse.hw_specs as hw_specs
    if getattr(hw_specs, "_ant_act_tables_patched", False):
        return
    orig = hw_specs.get_activation_tables

    @functools.cache
    def patched(module_arch):
        t = orig(module_arch)
        keep = "natural_log_exp_and_others"
        if keep not in t:
            return t
        drop = {mybir.ActivationFunctionType.Exp, mybir.ActivationFunctionType.Ln}
        return {name: (fns if name == keep else fns - drop)
                for name, fns in t.items()}

    hw_specs._ant_act_tables_patched = True
    hw_specs.get_activation_tables = patched
    import sys
    for modname in ("concourse.bacc", "concourse.bass_interp"):
        mod = sys.modules.get(modname)
        if mod is not None and hasattr(mod, "get_activation_tables"):
            mod.get_activation_tables = patched


_patch_act_tables()

AF = mybir.ActivationFunctionType
ALU = mybir.AluOpType
F32 = mybir.dt.float32
F16 = mybir.dt.float16
F8 = mybir.dt.float8e4
F8NP = mybir.dt.np(F8)
DR = mybir.MatmulPerfMode.DoubleRow
F8MAX = 240.0          # TRN fp8e4 saturation (not OCP's 448)
WS = 64.0              # fp8 FFN weight scale
HS = 16.0              # fp8 FFN input activation scale
OS = 32.0              # fp8 FFN hidden scale
WHS = WS * HS          # l1 psum descale
WOS = WS * OS          # l2 psum descale

B, S, D, FF, H, LE, LD, M = 16, 512, 512, 2048, 8, 6, 4, 2048
ROPE_BASE = 10000.0
LN_EPS = 1e-5
NCORES = 8
BL = B // NCORES          # 2 samples per core
T = BL * S                # 1024 tokens per core
KC = D // 128             # 4 feature chunks
FC = FF // 128            # 16
MC = M // 128             # 16
DH = D // H               # 64


# ----------------------------------------------------------------------------
# host-side helpers
# ----------------------------------------------------------------------------

def _chunked(wT):
    """[Din, O] -> [128, Din//128, O] contiguous fp16."""
    Din, O = wT.shape
    return np.ascontiguousarray(
        wT.reshape(Din // 128, 128, O).transpose(1, 0, 2)).astype(np.float16)


def _bias_cols(b):
    """[O] -> [128, O//128]  (column per 128-chunk), fp32."""
    O = b.shape[0]
    return np.ascontiguousarray(b.reshape(O // 128, 128).T).astype(np.float32)


def _chunked8(wT, scale=WS):
    """[Din, O] -> [128, Din//128, O] contiguous fp8e4, pre-scaled."""
    Din, O = wT.shape
    a = np.clip(wT * scale, -F8MAX, F8MAX)
    return np.ascontiguousarray(
        a.reshape(Din // 128, 128, O).transpose(1, 0, 2)).astype(F8NP)


_DEINT = np.concatenate([np.arange(0, D, 2), np.arange(1, D, 2)])  # de-interleave


def prep_weights(inp, le=LE, ld=LD):
    """Host-side weight prep -> dict of arrays shared by all cores."""
    out = {}
    g = {k: np.asarray(v, np.float64) for k, v in inp.items()}

    out["mpwT"] = np.ascontiguousarray(g["morph_proj_w"].T).astype(np.float16)
    out["mpb"] = _bias_cols(g["morph_proj_b"])                     # [128, 4]
    out["ppwT"] = np.ascontiguousarray(g["pose_proj_w"].T).astype(np.float16)
    out["ppb"] = _bias_cols(g["pose_proj_b"])

    # rope grids, de-interleaved frequency order: [128, 2, 512] fp16
    freq = 1.0 / ROPE_BASE ** (np.arange(0, D, 2, dtype=np.float64) / D)
    ang = np.outer(np.arange(S, dtype=np.float64), freq)           # [512, 256]
    cosT = np.cos(ang).T                                           # [256, S]
    sinT = np.sin(ang).T
    out["gridc"] = _chunked(cosT.reshape(256, S))
    out["grids"] = _chunked(sinT.reshape(256, S))

    e_w1, e_w2, e_wv, e_cqk, e_vb = [], [], [], [], []
    e_ow, e_owb, e_l1, e_l1b, e_l2, e_l2b = [], [], [], [], [], []
    for i in range(le):
        w1 = g["enc_in_w"][i] * g["enc_n1_g"][i][None, :]          # fold n1 g
        b1 = g["enc_in_b"][i] + g["enc_in_w"][i] @ g["enc_n1_b"][i]
        # stage-1 q,k only, de-interleaved output columns
        perm = np.concatenate([_DEINT, D + _DEINT])
        e_w1.append(_chunked(np.ascontiguousarray(w1[perm].T)))    # [128,4,1024]
        # stage-2 q,k (natural order, raw weights - the faithful quirk)
        w2 = g["enc_in_w"][i][: 2 * D]                             # Wq;Wk
        e_w2.append(_chunked(np.ascontiguousarray(w2.T)))          # [128,4,1024]
        # stage-2 bias tensors: C = W2 @ rope(b1) + b2   [512, S] each
        bq = b1[:D][_DEINT]                                        # [even; odd]
        bk = b1[D:2 * D][_DEINT]
        rb_q = np.concatenate([bq[:256, None] * cosT - bq[256:, None] * sinT,
                               bq[:256, None] * sinT + bq[256:, None] * cosT])
        rb_k = np.concatenate([bk[:256, None] * cosT - bk[256:, None] * sinT,
                               bk[:256, None] * sinT + bk[256:, None] * cosT])
        Cq = g["enc_in_w"][i][:D] @ rb_q + g["enc_in_b"][i][:D][:, None]
        Ck = g["enc_in_w"][i][D:2 * D] @ rb_k \
            + g["enc_in_b"][i][D:2 * D][:, None]
        C = np.concatenate([Cq, Ck], axis=0)                       # [1024, S]
        e_cqk.append(_chunked(C))                                  # [128,8,S]
        # V folded: V = h @ (Wv2 Wv1_f).T + (Wv2 bv1_f + bv2)
        Wv1f = w1[2 * D:]
        bv1f = b1[2 * D:]
        Wv2 = g["enc_in_w"][i][2 * D:]
        bv2 = g["enc_in_b"][i][2 * D:]
        e_wv.append(_chunked(np.ascontiguousarray((Wv2 @ Wv1f).T)))
        e_vb.append((Wv2 @ bv1f + bv2)[None, :].astype(np.float16))  # [1,512]
        e_ow.append(_chunked(np.ascontiguousarray(g["enc_out_w"][i].T)))
        e_owb.append(_bias_cols(g["enc_out_b"][i]))
        l1 = g["enc_l1_w"][i] * g["enc_n2_g"][i][None, :]
        l1b = g["enc_l1_b"][i] + g["enc_l1_w"][i] @ g["enc_n2_b"][i]
        e_l1.append(_chunked8(np.ascontiguousarray(l1.T)))         # [128,4,2048]
        e_l1b.append(_bias_cols(OS * l1b))                         # [128,16]
        e_l2.append(_chunked8(np.ascontiguousarray(g["enc_l2_w"][i].T)))
        e_l2b.append((WOS * g["enc_l2_b"][i])[None, :].astype(np.float16))
    out["ew1T"] = np.stack(e_w1) if le else np.zeros((0, 128, KC, 2 * D), np.float16)
    out["ew2T"] = np.stack(e_w2) if le else np.zeros((0, 128, KC, 2 * D), np.float16)
    out["ewvT"] = np.stack(e_wv) if le else np.zeros((0, 128, KC, D), np.float16)
    out["ecqk"] = np.stack(e_cqk) if le else np.zeros((0, 128, 8, S), np.float16)
    out["evb"] = np.stack(e_vb) if le else np.zeros((0, 1, D), np.float16)
    out["eowT"] = np.stack(e_ow) if le else np.zeros((0, 128, KC, D), np.float16)
    out["eowb"] = np.stack(e_owb) if le else np.zeros((0, 128, 4), np.float32)
    out["el1T"] = np.stack(e_l1) if le else np.zeros((0, 128, KC, FF), F8NP)
    out["el1b"] = np.stack(e_l1b) if le else np.zeros((0, 128, 16), np.float32)
    out["el2T"] = np.stack(e_l2) if le else np.zeros((0, 128, FC, D), F8NP)
    out["el2b"] = np.stack(e_l2b) if le else np.zeros((0, 1, D), np.float16)

    d_in, d_inb, d_vb, d_ow, d_owb = [], [], [], [], []
    d_m1, d_m1b, d_m2, d_m2b = [], [], [], []
    for i in range(ld):
        w = g["dec_in_w"][i].copy()
        b = g["dec_in_b"][i].copy()
        w[:D] = w[:D] * g["dec_n1_g"][i][None, :]                  # Wq <- dec_n1
        b[:D] = b[:D] + g["dec_in_w"][i][:D] @ g["dec_n1_b"][i]
        w[D:] = w[D:] * g["enc_final_g"][None, :]                  # Wk,Wv <- enc_final
        b[D:] = b[D:] + g["dec_in_w"][i][D:] @ g["enc_final_b"]
        d_in.append(_chunked(np.ascontiguousarray(w.T)))           # [128,4,1536]
        d_inb.append(_bias_cols(b))
        d_vb.append(b[2 * D:][None, :].astype(np.float16))         # [1,512]
        d_ow.append(_chunked(np.ascontiguousarray(g["dec_out_w"][i].T)))
        d_owb.append(_bias_cols(g["dec_out_b"][i]))
        m1 = g["dec_m1_w"][i] * g["dec_n2_g"][i][None, :]
        m1b = g["dec_m1_b"][i] + g["dec_m1_w"][i] @ g["dec_n2_b"][i]
        d_m1.append(_chunked(np.ascontiguousarray(m1.T)))          # [128,4,2048]
        d_m1b.append(_bias_cols(m1b))
        d_m2.append(_chunked(np.ascontiguousarray(g["dec_m2_w"][i].T)))
        d_m2b.append(_bias_cols(g["dec_m2_b"][i]))
    out["dinT"] = np.stack(d_in) if ld else np.zeros((0, 128, KC, 3 * D), np.float16)
    out["dinb"] = np.stack(d_inb) if ld else np.zeros((0, 128, 12), np.float32)
    out["dvb"] = np.stack(d_vb) if ld else np.zeros((0, 1, D), np.float16)
    out["dowT"] = np.stack(d_ow) if ld else np.zeros((0, 128, KC, D), np.float16)
    out["dowb"] = np.stack(d_owb) if ld else np.zeros((0, 128, 4), np.float32)
    out["dm1T"] = np.stack(d_m1) if ld else np.zeros((0, 128, KC, M), np.float16)
    out["dm1b"] = np.stack(d_m1b) if ld else np.zeros((0, 128, 16), np.float32)
    out["dm2T"] = np.stack(d_m2) if ld else np.zeros((0, 128, MC, D), np.float16)
    out["dm2b"] = np.stack(d_m2b) if ld else np.zeros((0, 128, 4), np.float32)

    hw = (g["head_w"] * g["head_g"][None, :])[0]                   # [512]
    out["hwT"] = _bias_cols(hw).astype(np.float16)                 # [128, 4]
    out["hb"] = (g["head_bias"] + g["head_w"] @ g["head_b"]).reshape(1, 1).astype(np.float32)
    return out


# ----------------------------------------------------------------------------
# device program
# ----------------------------------------------------------------------------

def build(le=LE, ld=LD):
    nc = bacc.Bacc(None, target_bir_lowering=False)

    dram = {}

    def din(name, shape, dt=F16):
        dram[name] = nc.dram_tensor(name, list(shape), dt, kind="ExternalInput")
        return dram[name]

    # shared weights
    din("mpwT", [3, 512]); din("mpb", [128, 4], F32)
    din("ppwT", [9, 512]); din("ppb", [128, 4], F32)
    din("gridc", [128, 2, S]); din("grids", [128, 2, S])
    din("ew1T", [le, 128, KC, 2 * D])
    din("ew2T", [le, 128, KC, 2 * D])
    din("ewvT", [le, 128, KC, D])
    din("ecqk", [le, 128, 8, S])
    din("evb", [le, 1, D])
    din("eowT", [le, 128, KC, D]); din("eowb", [le, 128, 4], F32)
    din("el1T", [le, 128, KC, FF], F8); din("el1b", [le, 128, 16], F32)
    din("el2T", [le, 128, FC, D], F8); din("el2b", [le, 1, D])
    din("dinT", [ld, 128, KC, 3 * D]); din("dinb", [ld, 128, 12], F32)
    din("dvb", [ld, 1, D])
    din("dowT", [ld, 128, KC, D]); din("dowb", [ld, 128, 4], F32)
    din("dm1T", [ld, 128, KC, M]); din("dm1b", [ld, 128, 16], F32)
    din("dm2T", [ld, 128, MC, D]); din("dm2b", [ld, 128, 4], F32)
    din("hwT", [128, KC]); din("hb", [1, 1], F32)
    # per-core inputs
    din("morphT", [3, T])
    din("poseT", [9, BL])
    y = nc.dram_tensor("y", [1, BL], F32, kind="ExternalOutput")

    with tile.TileContext(nc) as tc:
        _build_body(nc, tc, dram, y, le, ld)
    nc.compile()
    return nc


def _build_body(nc, tc, dram, y_dram, le, ld):
    import contextlib
    ctx = contextlib.ExitStack()
    with ctx:
        ctx.enter_context(nc.allow_low_precision(
            reason="fp16 matmul operands / activations are intentional"))
        persist = ctx.enter_context(tc.tile_pool(name="persist", bufs=1))
        wpool = ctx.enter_context(tc.tile_pool(name="wpool", bufs=3))
        w2pool = ctx.enter_context(tc.tile_pool(name="w2pool", bufs=1))
        owpool = ctx.enter_context(tc.tile_pool(name="owpool", bufs=1))
        vwpool = ctx.enter_context(tc.tile_pool(name="vwpool", bufs=1))
        cqpool = ctx.enter_context(tc.tile_pool(name="cqpool", bufs=1))
        bpool = ctx.enter_context(tc.tile_pool(name="bpool", bufs=2))
        a4 = ctx.enter_context(tc.tile_pool(name="a4", bufs=4))
        hp = ctx.enter_context(tc.tile_pool(name="hp", bufs=2))
        a8 = ctx.enter_context(tc.tile_pool(name="a8", bufs=3))
        qk2p = ctx.enter_context(tc.tile_pool(name="qk2p", bufs=1))
        vp = ctx.enter_context(tc.tile_pool(name="vp", bufs=2))
        rtp = ctx.enter_context(tc.tile_pool(name="rtp", bufs=3))
        vbp = ctx.enter_context(tc.tile_pool(name="vbp", bufs=1))
        scr = ctx.enter_context(tc.tile_pool(name="scr", bufs=3))
        invp = ctx.enter_context(tc.tile_pool(name="invp", bufs=2))
        rcp = ctx.enter_context(tc.tile_pool(name="rcp", bufs=2))
        smalls = ctx.enter_context(tc.tile_pool(name="smalls", bufs=2))
        b1 = ctx.enter_context(tc.tile_pool(name="b1", bufs=4, space="PSUM"))
        b2 = ctx.enter_context(tc.tile_pool(name="b2", bufs=4, space="PSUM"))

        # ---------------- persistent tiles ----------------
        x = persist.tile([128, KC, T], F16)           # residual stream (X.T)
        gridc = persist.tile([128, 2, S], F16)
        grids = persist.tile([128, 2, S], F16)
        ones128 = persist.tile([128, 1], F16)
        ones_row = persist.tile([1, 128], F16)
        ones_rowS = persist.tile([1, S], F16)         # bias-inject moving row
        hs_row = persist.tile([1, 128], F16)          # HS-scaled broadcast row
        ones8 = persist.tile([128, 8], F16)
        eps_t = persist.tile([1, 1], F32)
        p = persist.tile([128, KC, BL], F16)          # decoder latent p.T
        nc.sync.dma_start(gridc[:], dram["gridc"][:])
        nc.sync.dma_start(grids[:], dram["grids"][:])
        stage_f16 = rtp.tile([128, 128], F16, tag="rt")
        nc.vector.memset(stage_f16[:], 1.0)
        nc.vector.tensor_copy(ones128[:], stage_f16[:, 0:1])
        nc.vector.tensor_copy(ones_row[:], stage_f16[0:1, :])
        nc.vector.tensor_copy(ones8[:], stage_f16[:, 0:8])
        nc.vector.memset(hs_row[:], HS)
        nc.vector.memset(ones_rowS[:], 1.0)
        nc.vector.memset(eps_t[:], LN_EPS)

        def ln_small(x_tile, n_tok, h_out):
            """h_out = LayerNorm_features(x_tile) for tiny n_tok (decoder)."""
            sq = smalls.tile([128, KC, n_tok], F16, tag="sq_d")
            for k in range(KC):
                nc.vector.tensor_tensor(sq[:, k, :], x_tile[:, k, :],
                                        x_tile[:, k, :], ALU.mult)
            sum_ps = b2.tile([1, n_tok], F32, tag="b2")
            sq_ps = b2.tile([1, n_tok], F32, tag="b2")
            for k in range(KC):
                nc.tensor.matmul(sum_ps[:], ones128[:], x_tile[:, k, :],
                                 start=(k == 0), stop=(k == KC - 1))
            for k in range(KC):
                nc.tensor.matmul(sq_ps[:], ones128[:], sq[:, k, :],
                                 start=(k == 0), stop=(k == KC - 1))
            ms = scr.tile([1, n_tok], F32, tag="scr")
            t2 = scr.tile([1, n_tok], F32, tag="scr")
            rc = scr.tile([1, 2, n_tok], F16, tag="scr_rc")
            nc.scalar.activation(ms[:], sum_ps[:], AF.Copy, scale=1.0 / D)
            nc.vector.tensor_tensor(t2[:], ms[:], ms[:], ALU.mult)      # m^2
            nc.vector.scalar_tensor_tensor(
                t2[:], sq_ps[:], 1.0 / D, t2[:], ALU.mult, ALU.subtract)
            # r = exp(-0.5 ln(var + eps))
            nc.scalar.activation(t2[:], t2[:], AF.Ln, bias=eps_t[:])
            nc.scalar.activation(rc[:, 0, :], t2[:], AF.Exp, scale=-0.5)
            nc.vector.scalar_tensor_tensor(
                rc[:, 1, :], ms[:], -1.0, rc[:, 0, :], ALU.mult, ALU.mult)
            r_ps = b1.tile([128, n_tok], F32, tag="b1")
            c_ps = b1.tile([128, n_tok], F32, tag="b1")
            nc.tensor.matmul(r_ps[:], ones_row[:], rc[:, 0, :],
                             start=True, stop=True)
            nc.tensor.matmul(c_ps[:], ones_row[:], rc[:, 1, :],
                             start=True, stop=True)
            for k in range(KC):
                nc.vector.tensor_tensor(h_out[:, k, :], x_tile[:, k, :],
                                        r_ps[:], ALU.mult)
                nc.vector.tensor_tensor(h_out[:, k, :], h_out[:, k, :],
                                        c_ps[:], ALU.add)

        def ln_stats(x_tile, s):
            """Per-sample LN stats -> rc [1, 2, S] fp16 (r, c)."""
            sl = slice(s * S, (s + 1) * S)
            sq = a4.tile([128, KC, S], F16, tag="sq", bufs=2)
            nc.vector.tensor_tensor(sq[:], x_tile[:, :, sl], x_tile[:, :, sl],
                                    ALU.mult)
            sum_ps = b2.tile([1, S], F32, tag="b2")
            sq_ps = b2.tile([1, S], F32, tag="b2")
            for k in range(KC):
                nc.tensor.matmul(sum_ps[:], ones128[:], x_tile[:, k, sl],
                                 start=(k == 0), stop=(k == KC - 1))
            for k in range(KC):
                nc.tensor.matmul(sq_ps[:], ones128[:], sq[:, k, :],
                                 start=(k == 0), stop=(k == KC - 1))
            ms = scr.tile([1, S], F32, tag="scr")
            t2 = scr.tile([1, S], F32, tag="scr")
            rc = rcp.tile([1, 2, S], F16, tag="rc", bufs=4)
            nc.scalar.activation(ms[:], sum_ps[:], AF.Copy, scale=1.0 / D)
            nc.vector.tensor_tensor(t2[:], ms[:], ms[:], ALU.mult)
            nc.vector.scalar_tensor_tensor(
                t2[:], sq_ps[:], 1.0 / D, t2[:], ALU.mult, ALU.subtract)
            nc.scalar.activation(t2[:], t2[:], AF.Ln, bias=eps_t[:])
            nc.scalar.activation(rc[:, 0, :], t2[:], AF.Exp, scale=-0.5)
            nc.vector.scalar_tensor_tensor(
                rc[:, 1, :], ms[:], -1.0, rc[:, 0, :], ALU.mult, ALU.mult)
            return rc

        def ln_apply(rc, x_tile, s, h_out, scaled=False):
            """h_out = (x[:, :, s] * r + c) * (HS if scaled else 1)."""
            sl = slice(s * S, (s + 1) * S)
            row = hs_row if scaled else ones_row
            r_ps = b1.tile([128, S], F32, tag="b1")
            c_ps = b1.tile([128, S], F32, tag="b1")
            nc.tensor.matmul(r_ps[:], row[:], rc[:, 0, :],
                             start=True, stop=True)
            nc.tensor.matmul(c_ps[:], row[:], rc[:, 1, :],
                             start=True, stop=True)
            rcb = rcp.tile([128, 2, S], F16, tag="rcb")
            nc.scalar.activation(rcb[:, 0, :], r_ps[:], AF.Copy)
            nc.scalar.activation(rcb[:, 1, :], c_ps[:], AF.Copy)
            for k in range(KC):
                tmp = rtp.tile([128, S], F16, tag="rt")
                nc.vector.tensor_tensor(tmp[:], x_tile[:, k, sl],
                                        rcb[:, 0, :], ALU.mult)
                nc.vector.tensor_tensor(h_out[:, k, :], tmp[:],
                                        rcb[:, 1, :], ALU.add)

        # ---------------- morph projection -> x ----------------
        morpht = rtp.tile([3, T], F16, tag="morph", bufs=1)
        nc.sync.dma_start(morpht[:], dram["morphT"][:])
        mpw = rtp.tile([3, 512], F16, tag="rt")
        mpb = rtp.tile([128, 4], F32, tag="rtb")
        nc.sync.dma_start(mpw[:], dram["mpwT"][:])
        nc.sync.dma_start(mpb[:], dram["mpb"][:])
        rc1, hs_next = {}, {}
        for s in range(BL):
            for m in range(KC):
                ps = b1.tile([128, S], F32, tag="b1")
                nc.tensor.matmul(ps[:], mpw[:, m * 128:(m + 1) * 128],
                                 morpht[:, s * S:(s + 1) * S], start=True, stop=True)
                nc.scalar.activation(x[:, m, s * S:(s + 1) * S], ps[:], AF.Relu,
                                     bias=mpb[:, m:m + 1])
            rc1[s] = ln_stats(x, s)
            hs_next[s] = hp.tile([128, KC, S], F16, tag="h", name=f"h_pre{s}")
            ln_apply(rc1[s], x, s, hs_next[s])

        # ---------------- pose projection -> p ----------------
        poset = rtp.tile([9, BL], F16, tag="rt")
        ppw = rtp.tile([9, 512], F16, tag="rt")
        ppb = rtp.tile([128, 4], F32, tag="rtb")
        nc.sync.dma_start(poset[:], dram["poseT"][:])
        nc.sync.dma_start(ppw[:], dram["ppwT"][:])
        nc.sync.dma_start(ppb[:], dram["ppb"][:])
        pps = b1.tile([128, KC, BL], F32, tag="b1")
        for m in range(KC):
            nc.tensor.matmul(pps[:, m, :], ppw[:, m * 128:(m + 1) * 128],
                             poset[:], start=True, stop=True)
        for m in range(KC):
            nc.scalar.activation(p[:, m, :], pps[:, m, :], AF.Relu,
                                 bias=ppb[:, m:m + 1])

        # ---------------- encoder layers ----------------
        for li in range(le):
            w1 = wpool.tile([128, KC, 2 * D], F16, tag="bigw")
            nc.sync.dma_start(w1[:], dram["ew1T"][li])
            w2 = w2pool.tile([128, KC, 2 * D], F16, tag="w2")
            nc.sync.dma_start(w2[:], dram["ew2T"][li])
            wv = vwpool.tile([128, KC, D], F16, tag="wv")
            nc.sync.dma_start(wv[:], dram["ewvT"][li])
            cq = cqpool.tile([128, 8, S], F16, tag="cq")
            nc.sync.dma_start(cq[:], dram["ecqk"][li])
            vbrow = bpool.tile([1, D], F16, tag="vbrow")
            nc.sync.dma_start(vbrow[:], dram["evb"][li])
            ow = owpool.tile([128, KC, D], F16, tag="ow")
            nc.sync.dma_start(ow[:], dram["eowT"][li])
            owb = bpool.tile([128, 4], F32, tag="owb")
            nc.sync.dma_start(owb[:], dram["eowb"][li])
            l1 = wpool.tile([128, KC, FF], F8, tag="bigw")
            nc.sync.dma_start(l1[:], dram["el1T"][li])
            l1b = bpool.tile([128, 16], F32, tag="l1b")
            nc.sync.dma_start(l1b[:], dram["el1b"][li])
            l2b = bpool.tile([1, D], F16, tag="l2br")
            nc.sync.dma_start(l2b[:], dram["el2b"][li])

            # v-bias broadcast [128, 512], once per layer
            vb_ps = b1.tile([128, D], F32, tag="b1")
            nc.tensor.matmul(vb_ps[:], ones_row[:], vbrow[:], start=True, stop=True)
            vb_bc = vbp.tile([128, D], F16, tag="vb_bc")
            nc.scalar.activation(vb_bc[:], vb_ps[:], AF.Copy)

            qkrs, vlocs = {}, {}
            # ---- phase A1 per sample: stage1 q,k; V; rope ----
            # (LN1 stats AND apply were emitted at the end of the previous
            #  phase so the whole chain overlapped that phase's matmuls)
            for s in range(BL):
                h = hs_next[s]
                qkv1 = a8.tile([128, 8, S], F16, tag="a8")
                for m in range(8):
                    ps = b1.tile([128, S], F32, tag="b1")
                    for k in range(KC):
                        nc.tensor.matmul(ps[:], w1[:, k, m * 128:(m + 1) * 128],
                                         h[:, k, :], start=(k == 0), stop=(k == KC - 1))
                    nc.scalar.activation(qkv1[:, m, :], ps[:], AF.Copy)
                # V token-major (+ones col), from h directly (folded weights)
                vloc = vp.tile([128, KC, 8, 65], F16, tag="vloc")
                for t in range(KC):
                    nc.vector.tensor_copy(vloc[:, t, :, 64], ones8[:])
                for t in range(KC):
                    ps = b1.tile([128, S], F32, tag="b1")
                    for k in range(KC):
                        nc.tensor.matmul(
                            ps[:], h[:, k, t * 128:(t + 1) * 128],
                            wv[:, k, :], start=(k == 0), stop=(k == KC - 1))
                    nc.vector.tensor_tensor(
                        vloc[:, t, :, 0:64],
                        ps[:].rearrange("p (h d) -> p h d", h=H),
                        vb_bc[:].rearrange("p (h d) -> p h d", h=H), ALU.add)
                vlocs[s] = vloc
                # rope: qkv1 (de-interleaved) -> qkr (natural order), DVE fp16
                qkr = a8.tile([128, 8, S], F16, tag="a8")
                for half in (0, 4):
                    for c in range(2):
                        e = qkv1[:, half + c, :]
                        o = qkv1[:, half + 2 + c, :]
                        r1 = qkr[:, half + c, :]
                        r2 = qkr[:, half + 2 + c, :]
                        t1 = rtp.tile([128, S], F16, tag="rt")
                        nc.vector.tensor_tensor(r1, e, gridc[:, c, :], ALU.mult)
                        nc.vector.tensor_tensor(t1[:], o, grids[:, c, :], ALU.mult)
                        nc.vector.tensor_tensor(r1, r1, t1[:], ALU.subtract)
                        t2 = rtp.tile([128, S], F16, tag="rt")
                        nc.vector.tensor_tensor(r2, e, grids[:, c, :], ALU.mult)
                        nc.vector.tensor_tensor(t2[:], o, gridc[:, c, :], ALU.mult)
                        nc.vector.tensor_tensor(r2, r2, t2[:], ALU.add)
                qkrs[s] = qkr

            # ---- phase A2 per sample: stage2, attention, out-proj ----
            rc2, h2s = {}, {}
            for s in range(BL):
                sl = slice(s * S, (s + 1) * S)
                qkr, vloc = qkrs[s], vlocs[s]
                qk2 = qk2p.tile([128, 8, S], F16, tag="qk2")
                for m in range(8):
                    ps = b1.tile([128, S], F32, tag="b1")
                    base = 0 if m < 4 else 4
                    for k in range(KC):
                        nc.tensor.matmul(ps[:], w2[:, k, m * 128:(m + 1) * 128],
                                         qkr[:, base + k, :],
                                         start=(k == 0), stop=(k == KC - 1))
                    nc.vector.tensor_tensor(qk2[:, m, :], ps[:], cq[:, m, :],
                                            ALU.add)
                # attention heads, software-pipelined one pair ahead:
                # scores+exp of pair k+1 are emitted before A@V of pair k, so
                # the PE fills pair k's exp latency with pair k+1's scores.
                o_t = a4.tile([128, KC, S], F16, tag="ot", bufs=2)
                ats = {}

                def emit_scores(pair):
                    for hh in pair:
                        rows = slice(64 * (hh % 2), 64 * (hh % 2) + 64)
                        at = a4.tile([128, KC, S], F16, tag="at", bufs=4,
                                     name=f"at{hh}")
                        for c in range(KC):
                            scp = b1.tile([128, S], F32, tag="b1")
                            nc.tensor.matmul(
                                scp[:],
                                qk2[rows, 4 + hh // 2, c * 128:(c + 1) * 128],
                                qk2[rows, hh // 2, :], start=True, stop=True)
                            nc.scalar.activation(at[:, c, :], scp[:], AF.Exp,
                                                 scale=float(1.0 / np.sqrt(DH)))
                        ats[hh] = at

                pairs = [(0, 1), (2, 3), (4, 5), (6, 7)]
                emit_scores(pairs[0])
                for pi, pair in enumerate(pairs):
                    if pi + 1 < len(pairs):
                        emit_scores(pairs[pi + 1])
                    for hh in pair:
                        rows = slice(64 * (hh % 2), 64 * (hh % 2) + 64)
                        at = ats[hh]
                        ov = b2.tile([65, S], F32, tag="b2")
                        for c in range(KC):
                            nc.tensor.matmul(ov[:], vloc[:, c, hh, :], at[:, c, :],
                                             start=(c == 0), stop=(c == KC - 1))
                        # 1/denom = exp(-ln denom), broadcast via PE
                        lnd = scr.tile([1, S], F16, tag="lnd")
                        nc.scalar.activation(lnd[:], ov[64:65, :], AF.Ln)
                        rb = b2.tile([64, S], F32, tag="b2")
                        nc.tensor.matmul(rb[:], ones_row[:, 0:64], lnd[:],
                                         start=True, stop=True)
                        inv = invp.tile([64, S], F16, tag="inv")
                        nc.scalar.activation(inv[:], rb[:], AF.Exp, scale=-1.0)
                        nc.vector.tensor_tensor(o_t[rows, hh // 2, :],
                                                ov[0:64, :], inv[:], ALU.mult)
                # out-proj + residual
                for m in range(KC):
                    ps = b1.tile([128, S], F32, tag="b1")
                    for k in range(KC):
                        nc.tensor.matmul(ps[:], ow[:, k, m * 128:(m + 1) * 128],
                                         o_t[:, k, :], start=(k == 0),
                                         stop=(k == KC - 1))
                    nc.vector.scalar_tensor_tensor(
                        x[:, m, sl], ps[:], owb[:, m:m + 1], x[:, m, sl],
                        ALU.add, ALU.add)
                rc2[s] = ln_stats(x, s)
                h2s[s] = hp.tile([128, KC, S], F8, tag="h8", name=f"h2_{s}")
                ln_apply(rc2[s], x, s, h2s[s], scaled=True)

            # ---- phase B: FFN in fp8/DoubleRow (l2 streams in once w1 slot
            #      is released); biases ride in the PSUM via a rank-1 matmul.
            l2 = wpool.tile([128, FC, D], F8, tag="bigw")
            for kf in range(FC):
                nc.sync.dma_start(l2[:, kf, :], dram["el2T"][li][:, kf, :])
            for s in range(BL):
                sl = slice(s * S, (s + 1) * S)
                h2 = h2s[s]                               # h2 = HS*LN(x)
                f2 = [b1.tile([128, S], F32, tag="b1", name=f"f2_{_m}")
                      for _m in range(KC)]
                for m in range(KC):     # inject WOS*l2b into the accumulators
                    nc.tensor.matmul(f2[m][:], l2b[:, m * 128:(m + 1) * 128],
                                     ones_rowS[:], start=True, stop=False)
                for jf in range(FC // 2):
                    rt2 = rtp.tile([128, 2, S], F8, tag="rt8")
                    for i in range(2):
                        kf = 2 * jf + i
                        f1 = b2.tile([128, S], F32, tag="b2")
                        for c in range(2):
                            nc.tensor.matmul(
                                f1[:],
                                l1[:, 2 * c:2 * c + 2, kf * 128:(kf + 1) * 128],
                                h2[:, 2 * c:2 * c + 2, :],
                                start=(c == 0), stop=(c == 1), perf_mode=DR)
                        # rt = OS*relu(z + l1b):  f1 = WHS*z, bias = OS*l1b
                        nc.scalar.activation(rt2[:, i, :], f1[:], AF.Relu,
                                             scale=OS / WHS,
                                             bias=l1b[:, kf:kf + 1])
                    for m in range(KC):
                        nc.tensor.matmul(f2[m][:],
                                         l2[:, 2 * jf:2 * jf + 2, m * 128:(m + 1) * 128],
                                         rt2[:, :, :], start=False,
                                         stop=(jf == FC // 2 - 1), perf_mode=DR)
                for m in range(KC):
                    nc.vector.scalar_tensor_tensor(
                        x[:, m, sl], f2[m][:], 1.0 / WOS, x[:, m, sl],
                        ALU.mult, ALU.add)
                rc1[s] = ln_stats(x, s)   # next layer (or the final LN)
                if li + 1 < le:           # prefetch next layer's LN1 apply
                    hs_next[s] = hp.tile([128, KC, S], F16, tag="h",
                                         name=f"h_pre{s}")
                    ln_apply(rc1[s], x, s, hs_next[s])

        # ---------------- final encoder LN (in-place; affine folded) --------
        me = x
        for s in range(BL):
            ln_apply(rc1[s], x, s, x[:, :, s * S:(s + 1) * S])

        # ---------------- decoder layers ----------------
        for li in range(ld):
            dw = wpool.tile([128, KC, 3 * D], F16, tag="bigw")
            nc.sync.dma_start(dw[:], dram["dinT"][li])
            dwb = bpool.tile([128, 12], F32, tag="w1b")
            nc.sync.dma_start(dwb[:], dram["dinb"][li])
            dvbrow = bpool.tile([1, D], F16, tag="vbrow")
            nc.sync.dma_start(dvbrow[:], dram["dvb"][li])
            do = owpool.tile([128, KC, D], F16, tag="ow")
            nc.sync.dma_start(do[:], dram["dowT"][li])
            dob = bpool.tile([128, 4], F32, tag="owb")
            nc.sync.dma_start(dob[:], dram["dowb"][li])
            m1 = wpool.tile([128, KC, M], F16, tag="bigw")
            nc.sync.dma_start(m1[:], dram["dm1T"][li])
            m1b = bpool.tile([128, 16], F32, tag="l1b")
            nc.sync.dma_start(m1b[:], dram["dm1b"][li])
            m2b = bpool.tile([128, 4], F32, tag="l2b")
            nc.sync.dma_start(m2b[:], dram["dm2b"][li])

            vb_ps = b1.tile([128, D], F32, tag="b1")
            nc.tensor.matmul(vb_ps[:], ones_row[:], dvbrow[:], start=True, stop=True)
            vb_bc = vbp.tile([128, D], F16, tag="vb_bc")
            nc.scalar.activation(vb_bc[:], vb_ps[:], AF.Copy)

            # LN(p) -> q_ln ; Q projection (all samples at once, N=BL)
            q_ln = smalls.tile([128, KC, BL], F16, tag="q_ln")
            ln_small(p, BL, q_ln)
            qps = b1.tile([128, KC, BL], F32, tag="b1")
            for m in range(KC):
                for k in range(KC):
                    nc.tensor.matmul(qps[:, m, :],
                                     dw[:, k, m * 128:(m + 1) * 128],
                                     q_ln[:, k, :], start=(k == 0),
                                     stop=(k == KC - 1))
            q_sb = smalls.tile([128, KC, BL], F16, tag="q_sb")
            for m in range(KC):
                nc.scalar.activation(q_sb[:, m, :], qps[:, m, :], AF.Identity,
                                     bias=dwb[:, m:m + 1])
            o_d = smalls.tile([128, KC, BL], F16, tag="o_d")
            for s in range(BL):
                sl = slice(s * S, (s + 1) * S)
                # K (feature-major) and V' (token-major) over morph_enc
                k_sb = a4.tile([128, KC, S], F16, tag="at", bufs=4)
                for m in range(KC):
                    ps = b1.tile([128, S], F32, tag="b1")
                    for k in range(KC):
                        nc.tensor.matmul(
                            ps[:], dw[:, k, D + m * 128:D + (m + 1) * 128],
                            me[:, k, sl], start=(k == 0), stop=(k == KC - 1))
                    nc.scalar.activation(k_sb[:, m, :], ps[:], AF.Identity,
                                         bias=dwb[:, 4 + m:5 + m])
                vloc = vp.tile([128, KC, 8, 65], F16, tag="vloc")
                for t in range(KC):
                    nc.vector.tensor_copy(vloc[:, t, :, 64], ones8[:])
                for t in range(KC):
                    ps = b1.tile([128, S], F32, tag="b1")
                    for k in range(KC):
                        nc.tensor.matmul(
                            ps[:], me[:, k, s * S + t * 128:s * S + (t + 1) * 128],
                            dw[:, k, 2 * D:3 * D],
                            start=(k == 0), stop=(k == KC - 1))
                    nc.vector.tensor_tensor(
                        vloc[:, t, :, 0:64],
                        ps[:].rearrange("p (h d) -> p h d", h=H),
                        vb_bc[:].rearrange("p (h d) -> p h d", h=H), ALU.add)
                scp = b1.tile([128, KC, H], F32, tag="b1")
                for hh in range(H):
                    rows = slice(64 * (hh % 2), 64 * (hh % 2) + 64)
                    for c in range(KC):
                        nc.tensor.matmul(
                            scp[:, c, hh:hh + 1],
                            k_sb[rows, hh // 2, c * 128:(c + 1) * 128],
                            q_sb[rows, hh // 2, s:s + 1],
                            start=True, stop=True)
                at = smalls.tile([128, KC, H], F16, tag="at_d")
                nc.scalar.activation(at[:], scp[:], AF.Exp,
                                     scale=float(1.0 / np.sqrt(DH)))
                ov = b2.tile([65, H], F32, tag="b2")
                for hh in range(H):
                    for c in range(KC):
                        nc.tensor.matmul(ov[:, hh:hh + 1], vloc[:, c, hh, :],
                                         at[:, c, hh:hh + 1],
                                         start=(c == 0), stop=(c == KC - 1))
                # 1/denom = exp(-ln denom)
                lnd = scr.tile([1, H], F16, tag="lnd")
                nc.scalar.activation(lnd[:], ov[64:65, :], AF.Ln)
                rb = b2.tile([64, H], F32, tag="b2")
                nc.tensor.matmul(rb[:], ones_row[:, 0:64], lnd[:],
                                 start=True, stop=True)
                inv = invp.tile([64, H], F16, tag="inv_d")
                nc.scalar.activation(inv[:], rb[:], AF.Exp, scale=-1.0)
                for hh in range(H):
                    rows = slice(64 * (hh % 2), 64 * (hh % 2) + 64)
                    nc.vector.tensor_tensor(o_d[rows, hh // 2, s:s + 1],
                                            ov[0:64, hh:hh + 1],
                                            inv[:, hh:hh + 1], ALU.mult)
            # out-proj + residual into p
            ops = b1.tile([128, KC, BL], F32, tag="b1")
            for m in range(KC):
                for k in range(KC):
                    nc.tensor.matmul(ops[:, m, :],
                                     do[:, k, m * 128:(m + 1) * 128],
                                     o_d[:, k, :], start=(k == 0),
                                     stop=(k == KC - 1))
            for m in range(KC):
                nc.vector.scalar_tensor_tensor(
                    p[:, m, :], ops[:, m, :], dob[:, m:m + 1], p[:, m, :],
                    ALU.add, ALU.add)
            # FFN on p (m2 streams in chunked once dw releases its slot)
            m2 = wpool.tile([128, MC, D], F16, tag="bigw")
            for kf in range(MC):
                nc.sync.dma_start(m2[:, kf, :], dram["dm2T"][li][:, kf, :])
            h2d = smalls.tile([128, KC, BL], F16, tag="q_ln")
            ln_small(p, BL, h2d)
            mh = smalls.tile([128, MC, BL], F16, tag="mh")
            for mm_ in range(MC):
                ps = b1.tile([128, BL], F32, tag="b1")
                for k in range(KC):
                    nc.tensor.matmul(ps[:], m1[:, k, mm_ * 128:(mm_ + 1) * 128],
                                     h2d[:, k, :], start=(k == 0),
                                     stop=(k == KC - 1))
                nc.scalar.activation(mh[:, mm_, :], ps[:], AF.Relu,
                                     bias=m1b[:, mm_:mm_ + 1])
            m2ps = b1.tile([128, KC, BL], F32, tag="b1")
            for m in range(KC):
                for kf in range(MC):
                    nc.tensor.matmul(m2ps[:, m, :],
                                     m2[:, kf, m * 128:(m + 1) * 128],
                                     mh[:, kf, :], start=(kf == 0),
                                     stop=(kf == MC - 1))
            for m in range(KC):
                nc.vector.scalar_tensor_tensor(
                    p[:, m, :], m2ps[:, m, :], m2b[:, m:m + 1], p[:, m, :],
                    ALU.add, ALU.add)

        # ---------------- head ----------------
        hw = smalls.tile([128, KC], F16, tag="hw")
        hb = smalls.tile([1, 1], F32, tag="hb")
        nc.sync.dma_start(hw[:], dram["hwT"][:])
        nc.sync.dma_start(hb[:], dram["hb"][:])
        hg = smalls.tile([128, KC, BL], F16, tag="q_ln")
        ln_small(p, BL, hg)
        hps = b2.tile([1, BL], F32, tag="b2")
        for k in range(KC):
            nc.tensor.matmul(hps[:], hw[:, k:k + 1], hg[:, k, :],
                             start=(k == 0), stop=(k == KC - 1))
        y_sb = smalls.tile([1, BL], F32, tag="y_sb")
        nc.scalar.activation(y_sb[:], hps[:], AF.Sigmoid, bias=hb[:])
        nc.sync.dma_start(y_dram[:], y_sb[:])


# ----------------------------------------------------------------------------
# entry point
# ----------------------------------------------------------------------------

_NC_CACHE = {}


def kernel(**inputs):
    return _run(inputs, LE, LD)


def _run(inputs, le, ld, trace=False):
    w = prep_weights(inputs, le, ld)
    morph = np.asarray(inputs["morph"], np.float32)
    pose = np.asarray(inputs["pose"], np.float32)
    in_maps = []
    for c in range(NCORES):
        im = dict(w)
        mo = morph[c * BL:(c + 1) * BL]                 # [BL, S, 3]
        im["morphT"] = np.ascontiguousarray(
            mo.transpose(2, 0, 1).reshape(3, T)).astype(np.float16)
        im["poseT"] = np.ascontiguousarray(
            pose[c * BL:(c + 1) * BL].T).astype(np.float16)
        in_maps.append(im)

    if ("nc", le, ld) not in _NC_CACHE:
        _NC_CACHE[("nc", le, ld)] = build(le, ld)
    nc = _NC_CACHE[("nc", le, ld)]
    res = run_bass_kernel_spmd(nc, in_maps, core_ids=list(range(NCORES)),
                               trace=trace)
    out = np.zeros((B, 1), np.float32)
    for c in range(NCORES):
        out[c * BL:(c + 1) * BL, 0] = res.results[c]["y"][0]
    if trace:
        return out, res
    return out


# revision 60
# speedup vs baseline: 1.8791x; 1.0018x over previous
"""Trainium2 Bass kernel for nn_ReachabilityClassifierTransformer.

Data-parallel over batch: 16 samples / 8 cores = 2 samples per core.
Each core runs the full network (6-layer encoder + 4-layer decoder + head)
on its 2 samples. No collectives.

Pipeline (evolved from an fp32r baseline at 2.36 ms to ~1.26 ms):
  - fp16 matmul operands everywhere except the FFN (stationary weights get
    Fast-Weight-Load; DVE elementwise ops run in 2x mode; DMA halved).
    PSUM stays fp32.
  - Encoder FFN runs in fp8e4 with DoubleRow perf mode (2 contraction
    chunks per instruction).  Power-of-2 scales (weights x64, LN output
    x16, hidden x32) are folded into the LN broadcast row, the Relu
    activation scale, and the residual descale; l2's bias is injected into
    the PSUM accumulation by a rank-1 ones matmul so the residual stays a
    single DVE pass.
  - V path folded on host: V = h @ (Wv2 Wv1)^T + bv  (no rope between the
    two V projections, so the double-projection quirk collapses).
  - Stage-1 q/k biases folded through rope into per-position bias tensors
    C_q/C_k = W2 @ rope(b1) + b2, added at the stage-2 PSUM copy (rope is
    linear, rotation depends only on position).
  - No Sqrt / no DVE reciprocal anywhere: LN rsqrt = exp(-0.5 ln(v+eps)),
    softmax 1/denom = exp(-ln denom) broadcast via PE.  ln/exp/copy/relu/
    square all live in one activation table set (forced via
    _patch_act_tables) -> no table-set reloads.
  - LN stats AND the LN apply for the next phase are emitted at the end of
    the previous phase, so their scalar chains hide under matmuls; the
    attention loop is software-pipelined one head-pair ahead so each
    pair's exp latency is covered by the next pair's score matmuls.

Device layout conventions (per core):
  - Activations FEATURE-MAJOR in SBUF: tile [128, KC, T] holds X.T.
  - Weights pre-transposed on host to [in_feat, out_feat], laid out
    [128, KC_in, O] (partition = in-feature % 128).
  - matmul(out_psum[M,N], lhsT=[K,M], rhs=[K,N]) computes lhsT.T @ rhs.
  - Encoder stage-1 q,k output features are de-interleaved (even feats then
    odd feats) via host-side column permutation, so RoPE becomes contiguous
    block ops; the roped result is in natural (concatenated) order.
  - Softmax: scores computed transposed (S.T = K_h @ Q_h.T per chunk),
    exp'd without max subtraction (|scores/8| < 1 for this model), and the
    denominator comes free from a ones-column appended to V.
"""
import functools

import numpy as np

import concourse.bass as bass
import concourse.mybir as mybir
import concourse.tile as tile
from concourse import bacc
from concourse.bass_utils import run_bass_kernel_spmd


def _patch_act_tables():
    """Constrain exp/ln to the one table set that contains both.

    The act-table-load pass maps each activation function to a set
    independently (exp -> exp_and_others, ln -> natural_log), so a kernel
    that interleaves exp and ln reloads tables on every transition
    (~1.3us each).  natural_log_exp_and_others contains exp AND ln (plus
    copy/identity/relu/square), so restricting exp/ln to that set makes
    every load resolve there; set ids/order are preserved so the emitted
    act_func_set_id still indexes the real act_info.json.
    """
    import concourse.hw_specs as hw_specs
    if getattr(hw_specs, "_ant_act_tables_patched", False):
        return
    orig = hw_specs.get_activation_tables

    @functools.cache
    def patched(module_arch):
        t = orig(module_arch)
        keep = "natural_log_exp_and_others"
        if keep not in t:
            return t
        drop = {mybir.ActivationFunctionType.Exp, mybir.ActivationFunctionType.Ln}
        return {name: (fns if name == keep else fns - drop)
                for name, fns in t.items()}

    hw_specs._ant_act_tables_patched = True
    hw_specs.get_activation_tables = patched
    import sys
    for modname in ("concourse.bacc", "concourse.bass_interp"):
        mod = sys.modules.get(modname)
        if mod is not None and hasattr(mod, "get_activation_tables"):
            mod.get_activation_tables = patched


_patch_act_tables()

AF = mybir.ActivationFunctionType
ALU = mybir.AluOpType
F32 = mybir.dt.float32
F16 = mybir.dt.float16
F8 = mybir.dt.float8e4
F8NP = mybir.dt.np(F8)
DR = mybir.MatmulPerfMode.DoubleRow
F8MAX = 240.0          # TRN fp8e4 saturation (not OCP's 448)
WS = 64.0              # fp8 FFN weight scale
HS = 16.0              # fp8 FFN input activation scale
OS = 32.0              # fp8 FFN hidden scale
WHS = WS * HS          # l1 psum descale
WOS = WS * OS          # l2 psum descale

B, S, D, FF, H, LE, LD, M = 16, 512, 512, 2048, 8, 6, 4, 2048
ROPE_BASE = 10000.0
LN_EPS = 1e-5
NCORES = 8
BL = B // NCORES          # 2 samples per core
T = BL * S                # 1024 tokens per core
KC = D // 128             # 4 feature chunks
FC = FF // 128            # 16
MC = M // 128             # 16
DH = D // H               # 64


# ----------------------------------------------------------------------------
# host-side helpers
# ----------------------------------------------------------------------------

def _chunked(wT):
    """[Din, O] -> [128, Din//128, O] contiguous fp16."""
    Din, O = wT.shape
    return np.ascontiguousarray(
        wT.reshape(Din // 128, 128, O).transpose(1, 0, 2)).astype(np.float16)


def _bias_cols(b):
    """[O] -> [128, O//128]  (column per 128-chunk), fp32."""
    O = b.shape[0]
    return np.ascontiguousarray(b.reshape(O // 128, 128).T).astype(np.float32)


def _chunked8(wT, scale=WS):
    """[Din, O] -> [128, Din//128, O] contiguous fp8e4, pre-scaled."""
    Din, O = wT.shape
    a = np.clip(wT * scale, -F8MAX, F8MAX)
    return np.ascontiguousarray(
        a.reshape(Din // 128, 128, O).transpose(1, 0, 2)).astype(F8NP)


_DEINT = np.concatenate([np.arange(0, D, 2), np.arange(1, D, 2)])  # de-interleave


def prep_weights(inp, le=LE, ld=LD):
    """Host-side weight prep -> dict of arrays shared by all cores."""
    out = {}
    g = {k: np.asarray(v, np.float64) for k, v in inp.items()}

    out["mpwT"] = np.ascontiguousarray(g["morph_proj_w"].T).astype(np.float16)
    out["mpb"] = _bias_cols(g["morph_proj_b"])                     # [128, 4]
    out["ppwT"] = np.ascontiguousarray(g["pose_proj_w"].T).astype(np.float16)
    out["ppb"] = _bias_cols(g["pose_proj_b"])

    # rope grids, de-interleaved frequency order: [128, 2, 512] fp16
    freq = 1.0 / ROPE_BASE ** (np.arange(0, D, 2, dtype=np.float64) / D)
    ang = np.outer(np.arange(S, dtype=np.float64), freq)           # [512, 256]
    cosT = np.cos(ang).T                                           # [256, S]
    sinT = np.sin(ang).T
    out["gridc"] = _chunked(cosT.reshape(256, S))
    out["grids"] = _chunked(sinT.reshape(256, S))

    e_w1, e_w2, e_wv, e_cqk, e_vb = [], [], [], [], []
    e_ow, e_owb, e_l1, e_l1b, e_l2, e_l2b = [], [], [], [], [], []
    for i in range(le):
        w1 = g["enc_in_w"][i] * g["enc_n1_g"][i][None, :]          # fold n1 g
        b1 = g["enc_in_b"][i] + g["enc_in_w"][i] @ g["enc_n1_b"][i]
        # stage-1 q,k only, de-interleaved output columns
        perm = np.concatenate([_DEINT, D + _DEINT])
        e_w1.append(_chunked(np.ascontiguousarray(w1[perm].T)))    # [128,4,1024]
        # stage-2 q,k (natural order, raw weights - the faithful quirk)
        w2 = g["enc_in_w"][i][: 2 * D]                             # Wq;Wk
        e_w2.append(_chunked(np.ascontiguousarray(w2.T)))          # [128,4,1024]
        # stage-2 bias tensors: C = W2 @ rope(b1) + b2   [512, S] each
        bq = b1[:D][_DEINT]                                        # [even; odd]
        bk = b1[D:2 * D][_DEINT]
        rb_q = np.concatenate([bq[:256, None] * cosT - bq[256:, None] * sinT,
                               bq[:256, None] * sinT + bq[256:, None] * cosT])
        rb_k = np.concatenate([bk[:256, None] * cosT - bk[256:, None] * sinT,
                               bk[:256, None] * sinT + bk[256:, None] * cosT])
        Cq = g["enc_in_w"][i][:D] @ rb_q + g["enc_in_b"][i][:D][:, None]
        Ck = g["enc_in_w"][i][D:2 * D] @ rb_k \
            + g["enc_in_b"][i][D:2 * D][:, None]
        C = np.concatenate([Cq, Ck], axis=0)                       # [1024, S]
        e_cqk.append(_chunked(C))                                  # [128,8,S]
        # V folded: V = h @ (Wv2 Wv1_f).T + (Wv2 bv1_f + bv2)
        Wv1f = w1[2 * D:]
        bv1f = b1[2 * D:]
        Wv2 = g["enc_in_w"][i][2 * D:]
        bv2 = g["enc_in_b"][i][2 * D:]
        e_wv.append(_chunked(np.ascontiguousarray((Wv2 @ Wv1f).T)))
        e_vb.append((Wv2 @ bv1f + bv2)[None, :].astype(np.float16))  # [1,512]
        e_ow.append(_chunked(np.ascontiguousarray(g["enc_out_w"][i].T)))
        e_owb.append(_bias_cols(g["enc_out_b"][i]))
        l1 = g["enc_l1_w"][i] * g["enc_n2_g"][i][None, :]
        l1b = g["enc_l1_b"][i] + g["enc_l1_w"][i] @ g["enc_n2_b"][i]
        e_l1.append(_chunked8(np.ascontiguousarray(l1.T)))         # [128,4,2048]
        e_l1b.append(_bias_cols(OS * l1b))                         # [128,16]
        e_l2.append(_chunked8(np.ascontiguousarray(g["enc_l2_w"][i].T)))
        e_l2b.append((WOS * g["enc_l2_b"][i])[None, :].astype(np.float16))
    out["ew1T"] = np.stack(e_w1) if le else np.zeros((0, 128, KC, 2 * D), np.float16)
    out["ew2T"] = np.stack(e_w2) if le else np.zeros((0, 128, KC, 2 * D), np.float16)
    out["ewvT"] = np.stack(e_wv) if le else np.zeros((0, 128, KC, D), np.float16)
    out["ecqk"] = np.stack(e_cqk) if le else np.zeros((0, 128, 8, S), np.float16)
    out["evb"] = np.stack(e_vb) if le else np.zeros((0, 1, D), np.float16)
    out["eowT"] = np.stack(e_ow) if le else np.zeros((0, 128, KC, D), np.float16)
    out["eowb"] = np.stack(e_owb) if le else np.zeros((0, 128, 4), np.float32)
    out["el1T"] = np.stack(e_l1) if le else np.zeros((0, 128, KC, FF), F8NP)
    out["el1b"] = np.stack(e_l1b) if le else np.zeros((0, 128, 16), np.float32)
    out["el2T"] = np.stack(e_l2) if le else np.zeros((0, 128, FC, D), F8NP)
    out["el2b"] = np.stack(e_l2b) if le else np.zeros((0, 1, D), np.float16)

    d_in, d_inb, d_vb, d_ow, d_owb = [], [], [], [], []
    d_m1, d_m1b, d_m2, d_m2b = [], [], [], []
    for i in range(ld):
        w = g["dec_in_w"][i].copy()
        b = g["dec_in_b"][i].copy()
        w[:D] = w[:D] * g["dec_n1_g"][i][None, :]                  # Wq <- dec_n1
        b[:D] = b[:D] + g["dec_in_w"][i][:D] @ g["dec_n1_b"][i]
        w[D:] = w[D:] * g["enc_final_g"][None, :]                  # Wk,Wv <- enc_final
        b[D:] = b[D:] + g["dec_in_w"][i][D:] @ g["enc_final_b"]
        d_in.append(_chunked(np.ascontiguousarray(w.T)))           # [128,4,1536]
        d_inb.append(_bias_cols(b))
        d_vb.append(b[2 * D:][None, :].astype(np.float16))         # [1,512]
        d_ow.append(_chunked(np.ascontiguousarray(g["dec_out_w"][i].T)))
        d_owb.append(_bias_cols(g["dec_out_b"][i]))
        m1 = g["dec_m1_w"][i] * g["dec_n2_g"][i][None, :]
        m1b = g["dec_m1_b"][i] + g["dec_m1_w"][i] @ g["dec_n2_b"][i]
        d_m1.append(_chunked(np.ascontiguousarray(m1.T)))          # [128,4,2048]
        d_m1b.append(_bias_cols(m1b))
        d_m2.append(_chunked(np.ascontiguousarray(g["dec_m2_w"][i].T)))
        d_m2b.append(_bias_cols(g["dec_m2_b"][i]))
    out["dinT"] = np.stack(d_in) if ld else np.zeros((0, 128, KC, 3 * D), np.float16)
    out["dinb"] = np.stack(d_inb) if ld else np.zeros((0, 128, 12), np.float32)
    out["dvb"] = np.stack(d_vb) if ld else np.zeros((0, 1, D), np.float16)
    out["dowT"] = np.stack(d_ow) if ld else np.zeros((0, 128, KC, D), np.float16)
    out["dowb"] = np.stack(d_owb) if ld else np.zeros((0, 128, 4), np.float32)
    out["dm1T"] = np.stack(d_m1) if ld else np.zeros((0, 128, KC, M), np.float16)
    out["dm1b"] = np.stack(d_m1b) if ld else np.zeros((0, 128, 16), np.float32)
    out["dm2T"] = np.stack(d_m2) if ld else np.zeros((0, 128, MC, D), np.float16)
    out["dm2b"] = np.stack(d_m2b) if ld else np.zeros((0, 128, 4), np.float32)

    hw = (g["head_w"] * g["head_g"][None, :])[0]                   # [512]
    out["hwT"] = _bias_cols(hw).astype(np.float16)                 # [128, 4]
    out["hb"] = (g["head_bias"] + g["head_w"] @ g["head_b"]).reshape(1, 1).astype(np.float32)
    return out


# ----------------------------------------------------------------------------
# device program
# ----------------------------------------------------------------------------

def build(le=LE, ld=LD):
    nc = bacc.Bacc(None, target_bir_lowering=False)

    dram = {}

    def din(name, shape, dt=F16):
        dram[name] = nc.dram_tensor(name, list(shape), dt, kind="ExternalInput")
        return dram[name]

    # shared weights
    din("mpwT", [3, 512]); din("mpb", [128, 4], F32)
    din("ppwT", [9, 512]); din("ppb", [128, 4], F32)
    din("gridc", [128, 2, S]); din("grids", [128, 2, S])
    din("ew1T", [le, 128, KC, 2 * D])
    din("ew2T", [le, 128, KC, 2 * D])
    din("ewvT", [le, 128, KC, D])
    din("ecqk", [le, 128, 8, S])
    din("evb", [le, 1, D])
    din("eowT", [le, 128, KC, D]); din("eowb", [le, 128, 4], F32)
    din("el1T", [le, 128, KC, FF], F8); din("el1b", [le, 128, 16], F32)
    din("el2T", [le, 128, FC, D], F8); din("el2b", [le, 1, D])
    din("dinT", [ld, 128, KC, 3 * D]); din("dinb", [ld, 128, 12], F32)
    din("dvb", [ld, 1, D])
    din("dowT", [ld, 128, KC, D]); din("dowb", [ld, 128, 4], F32)
    din("dm1T", [ld, 128, KC, M]); din("dm1b", [ld, 128, 16], F32)
    din("dm2T", [ld, 128, MC, D]); din("dm2b", [ld, 128, 4], F32)
    din("hwT", [128, KC]); din("hb", [1, 1], F32)
    # per-core inputs
    din("morphT", [3, T])
    din("poseT", [9, BL])
    y = nc.dram_tensor("y", [1, BL], F32, kind="ExternalOutput")

    with tile.TileContext(nc) as tc:
        _build_body(nc, tc, dram, y, le, ld)
    nc.compile()
    return nc


def _build_body(nc, tc, dram, y_dram, le, ld):
    import contextlib
    ctx = contextlib.ExitStack()
    with ctx:
        ctx.enter_context(nc.allow_low_precision(
            reason="fp16 matmul operands / activations are intentional"))
        persist = ctx.enter_context(tc.tile_pool(name="persist", bufs=1))
        wpool = ctx.enter_context(tc.tile_pool(name="wpool", bufs=3))
        w2pool = ctx.enter_context(tc.tile_pool(name="w2pool", bufs=1))
        owpool = ctx.enter_context(tc.tile_pool(name="owpool", bufs=1))
        vwpool = ctx.enter_context(tc.tile_pool(name="vwpool", bufs=1))
        cqpool = ctx.enter_context(tc.tile_pool(name="cqpool", bufs=1))
        bpool = ctx.enter_context(tc.tile_pool(name="bpool", bufs=2))
        a4 = ctx.enter_context(tc.tile_pool(name="a4", bufs=4))
        hp = ctx.enter_context(tc.tile_pool(name="hp", bufs=2))
        a8 = ctx.enter_context(tc.tile_pool(name="a8", bufs=3))
        qk2p = ctx.enter_context(tc.tile_pool(name="qk2p", bufs=1))
        vp = ctx.enter_context(tc.tile_pool(name="vp", bufs=2))
        rtp = ctx.enter_context(tc.tile_pool(name="rtp", bufs=3))
        vbp = ctx.enter_context(tc.tile_pool(name="vbp", bufs=1))
        scr = ctx.enter_context(tc.tile_pool(name="scr", bufs=3))
        invp = ctx.enter_context(tc.tile_pool(name="invp", bufs=2))
        rcp = ctx.enter_context(tc.tile_pool(name="rcp", bufs=2))
        smalls = ctx.enter_context(tc.tile_pool(name="smalls", bufs=2))
        b1 = ctx.enter_context(tc.tile_pool(name="b1", bufs=4, space="PSUM"))
        b2 = ctx.enter_context(tc.tile_pool(name="b2", bufs=4, space="PSUM"))

        # ---------------- persistent tiles ----------------
        x = persist.tile([128, KC, T], F16)           # residual stream (X.T)
        gridc = persist.tile([128, 2, S], F16)
        grids = persist.tile([128, 2, S], F16)
        ones128 = persist.tile([128, 1], F16)
        ones_row = persist.tile([1, 128], F16)
        ones_rowS = persist.tile([1, S], F16)         # bias-inject moving row
        hs_row = persist.tile([1, 128], F16)          # HS-scaled broadcast row
        ones8 = persist.tile([128, 8], F16)
        eps_t = persist.tile([1, 1], F32)
        p = persist.tile([128, KC, BL], F16)          # decoder latent p.T
        nc.sync.dma_start(gridc[:], dram["gridc"][:])
        nc.sync.dma_start(grids[:], dram["grids"][:])
        stage_f16 = rtp.tile([128, 128], F16, tag="rt")
        nc.vector.memset(stage_f16[:], 1.0)
        nc.vector.tensor_copy(ones128[:], stage_f16[:, 0:1])
        nc.vector.tensor_copy(ones_row[:], stage_f16[0:1, :])
        nc.vector.tensor_copy(ones8[:], stage_f16[:, 0:8])
        nc.vector.memset(hs_row[:], HS)
        nc.vector.memset(ones_rowS[:], 1.0)
        nc.vector.memset(eps_t[:], LN_EPS)

        def ln_small(x_tile, n_tok, h_out):
            """h_out = LayerNorm_features(x_tile) for tiny n_tok (decoder)."""
            sq = smalls.tile([128, KC, n_tok], F16, tag="sq_d")
            for k in range(KC):
                nc.vector.tensor_tensor(sq[:, k, :], x_tile[:, k, :],
                                        x_tile[:, k, :], ALU.mult)
            sum_ps = b2.tile([1, n_tok], F32, tag="b2")
            sq_ps = b2.tile([1, n_tok], F32, tag="b2")
            for k in range(KC):
                nc.tensor.matmul(sum_ps[:], ones128[:], x_tile[:, k, :],
                                 start=(k == 0), stop=(k == KC - 1))
            for k in range(KC):
                nc.tensor.matmul(sq_ps[:], ones128[:], sq[:, k, :],
                                 start=(k == 0), stop=(k == KC - 1))
            ms = scr.tile([1, n_tok], F32, tag="scr")
            t2 = scr.tile([1, n_tok], F32, tag="scr")
            rc = scr.tile([1, 2, n_tok], F16, tag="scr_rc")
            nc.scalar.activation(ms[:], sum_ps[:], AF.Copy, scale=1.0 / D)
            nc.vector.tensor_tensor(t2[:], ms[:], ms[:], ALU.mult)      # m^2
            nc.vector.scalar_tensor_tensor(
                t2[:], sq_ps[:], 1.0 / D, t2[:], ALU.mult, ALU.subtract)
            # r = exp(-0.5 ln(var + eps))
            nc.scalar.activation(t2[:], t2[:], AF.Ln, bias=eps_t[:])
            nc.scalar.activation(rc[:, 0, :], t2[:], AF.Exp, scale=-0.5)
            nc.vector.scalar_tensor_tensor(
                rc[:, 1, :], ms[:], -1.0, rc[:, 0, :], ALU.mult, ALU.mult)
            r_ps = b1.tile([128, n_tok], F32, tag="b1")
            c_ps = b1.tile([128, n_tok], F32, tag="b1")
            nc.tensor.matmul(r_ps[:], ones_row[:], rc[:, 0, :],
                             start=True, stop=True)
            nc.tensor.matmul(c_ps[:], ones_row[:], rc[:, 1, :],
                             start=True, stop=True)
            for k in range(KC):
                nc.vector.tensor_tensor(h_out[:, k, :], x_tile[:, k, :],
                                        r_ps[:], ALU.mult)
                nc.vector.tensor_tensor(h_out[:, k, :], h_out[:, k, :],
                                        c_ps[:], ALU.add)

        def ln_stats(x_tile, s):
            """Per-sample LN stats -> rc [1, 2, S] fp16 (r, c)."""
            sl = slice(s * S, (s + 1) * S)
            sq = a4.tile([128, KC, S], F16, tag="sq", bufs=2)
            nc.vector.tensor_tensor(sq[:], x_tile[:, :, sl], x_tile[:, :, sl],
                                    ALU.mult)
            sum_ps = b2.tile([1, S], F32, tag="b2")
            sq_ps = b2.tile([1, S], F32, tag="b2")
            for k in range(KC):
                nc.tensor.matmul(sum_ps[:], ones128[:], x_tile[:, k, sl],
                                 start=(k == 0), stop=(k == KC - 1))
            for k in range(KC):
                nc.tensor.matmul(sq_ps[:], ones128[:], sq[:, k, :],
                                 start=(k == 0), stop=(k == KC - 1))
            ms = scr.tile([1, S], F32, tag="scr")
            t2 = scr.tile([1, S], F32, tag="scr")
            rc = rcp.tile([1, 2, S], F16, tag="rc", bufs=4)
            nc.scalar.activation(ms[:], sum_ps[:], AF.Copy, scale=1.0 / D)
            nc.vector.tensor_tensor(t2[:], ms[:], ms[:], ALU.mult)
            nc.vector.scalar_tensor_tensor(
                t2[:], sq_ps[:], 1.0 / D, t2[:], ALU.mult, ALU.subtract)
            nc.scalar.activation(t2[:], t2[:], AF.Ln, bias=eps_t[:])
            nc.scalar.activation(rc[:, 0, :], t2[:], AF.Exp, scale=-0.5)
            nc.vector.scalar_tensor_tensor(
                rc[:, 1, :], ms[:], -1.0, rc[:, 0, :], ALU.mult, ALU.mult)
            return rc

        def ln_apply(rc, x_tile, s, h_out, scaled=False):
            """h_out = (x[:, :, s] * r + c) * (HS if scaled else 1)."""
            sl = slice(s * S, (s + 1) * S)
            row = hs_row if scaled else ones_row
            r_ps = b1.tile([128, S], F32, tag="b1")
            c_ps = b1.tile([128, S], F32, tag="b1")
            nc.tensor.matmul(r_ps[:], row[:], rc[:, 0, :],
                             start=True, stop=True)
            nc.tensor.matmul(c_ps[:], row[:], rc[:, 1, :],
                             start=True, stop=True)
            rcb = rcp.tile([128, 2, S], F16, tag="rcb")
            nc.scalar.activation(rcb[:, 0, :], r_ps[:], AF.Copy)
            nc.scalar.activation(rcb[:, 1, :], c_ps[:], AF.Copy)
            for k in range(KC):
                tmp = rtp.tile([128, S], F16, tag="rt")
                nc.vector.tensor_tensor(tmp[:], x_tile[:, k, sl],
                                        rcb[:, 0, :], ALU.mult)
                nc.vector.tensor_tensor(h_out[:, k, :], tmp[:],
                                        rcb[:, 1, :], ALU.add)

        # ---------------- morph projection -> x ----------------
        morpht = rtp.tile([3, T], F16, tag="morph", bufs=1)
        nc.sync.dma_start(morpht[:], dram["morphT"][:])
        mpw = rtp.tile([3, 512], F16, tag="rt")
        mpb = rtp.tile([128, 4], F32, tag="rtb")
        nc.sync.dma_start(mpw[:], dram["mpwT"][:])
        nc.sync.dma_start(mpb[:], dram["mpb"][:])
        rc1, hs_next = {}, {}
        for s in range(BL):
            for m in range(KC):
                ps = b1.tile([128, S], F32, tag="b1")
                nc.tensor.matmul(ps[:], mpw[:, m * 128:(m + 1) * 128],
                                 morpht[:, s * S:(s + 1) * S], start=True, stop=True)
                nc.scalar.activation(x[:, m, s * S:(s + 1) * S], ps[:], AF.Relu,
                                     bias=mpb[:, m:m + 1])
            rc1[s] = ln_stats(x, s)
            hs_next[s] = hp.tile([128, KC, S], F16, tag="h", name=f"h_pre{s}")
            ln_apply(rc1[s], x, s, hs_next[s])

        # ---------------- pose projection -> p ----------------
        poset = rtp.tile([9, BL], F16, tag="rt")
        ppw = rtp.tile([9, 512], F16, tag="rt")
        ppb = rtp.tile([128, 4], F32, tag="rtb")
        nc.sync.dma_start(poset[:], dram["poseT"][:])
        nc.sync.dma_start(ppw[:], dram["ppwT"][:])
        nc.sync.dma_start(ppb[:], dram["ppb"][:])
        pps = b1.tile([128, KC, BL], F32, tag="b1")
        for m in range(KC):
            nc.tensor.matmul(pps[:, m, :], ppw[:, m * 128:(m + 1) * 128],
                             poset[:], start=True, stop=True)
        for m in range(KC):
            nc.scalar.activation(p[:, m, :], pps[:, m, :], AF.Relu,
                                 bias=ppb[:, m:m + 1])

        # ---------------- encoder layers ----------------
        for li in range(le):
            w1 = wpool.tile([128, KC, 2 * D], F16, tag="bigw")
            nc.sync.dma_start(w1[:], dram["ew1T"][li])
            w2 = w2pool.tile([128, KC, 2 * D], F16, tag="w2")
            nc.sync.dma_start(w2[:], dram["ew2T"][li])
            wv = vwpool.tile([128, KC, D], F16, tag="wv")
            nc.sync.dma_start(wv[:], dram["ewvT"][li])
            cq = cqpool.tile([128, 8, S], F16, tag="cq")
            nc.sync.dma_start(cq[:], dram["ecqk"][li])
            vbrow = bpool.tile([1, D], F16, tag="vbrow")
            nc.sync.dma_start(vbrow[:], dram["evb"][li])
            ow = owpool.tile([128, KC, D], F16, tag="ow")
            nc.sync.dma_start(ow[:], dram["eowT"][li])
            owb = bpool.tile([128, 4], F32, tag="owb")
            nc.sync.dma_start(owb[:], dram["eowb"][li])
            l1 = wpool.tile([128, KC, FF], F8, tag="bigw")
            nc.sync.dma_start(l1[:], dram["el1T"][li])
            l1b = bpool.tile([128, 16], F32, tag="l1b")
            nc.sync.dma_start(l1b[:], dram["el1b"][li])
            l2b = bpool.tile([1, D], F16, tag="l2br")
            nc.sync.dma_start(l2b[:], dram["el2b"][li])

            # v-bias broadcast [128, 512], once per layer
            vb_ps = b1.tile([128, D], F32, tag="b1")
            nc.tensor.matmul(vb_ps[:], ones_row[:], vbrow[:], start=True, stop=True)
            vb_bc = vbp.tile([128, D], F16, tag="vb_bc")
            nc.scalar.activation(vb_bc[:], vb_ps[:], AF.Copy)

            qkrs, vlocs = {}, {}
            # ---- phase A1 per sample: stage1 q,k; V; rope ----
            # (LN1 stats AND apply were emitted at the end of the previous
            #  phase so the whole chain overlapped that phase's matmuls)
            for s in range(BL):
                h = hs_next[s]
                qkv1 = a8.tile([128, 8, S], F16, tag="a8")
                for m in range(8):
                    ps = b1.tile([128, S], F32, tag="b1")
                    for k in range(KC):
                        nc.tensor.matmul(ps[:], w1[:, k, m * 128:(m + 1) * 128],
                                         h[:, k, :], start=(k == 0), stop=(k == KC - 1))
                    nc.scalar.activation(qkv1[:, m, :], ps[:], AF.Copy)
                # V token-major (+ones col), from h directly (folded weights)
                vloc = vp.tile([128, KC, 8, 65], F16, tag="vloc")
                for t in range(KC):
                    nc.vector.tensor_copy(vloc[:, t, :, 64], ones8[:])
                for t in range(KC):
                    ps = b1.tile([128, S], F32, tag="b1")
                    for k in range(KC):
                        nc.tensor.matmul(
                            ps[:], h[:, k, t * 128:(t + 1) * 128],
                            wv[:, k, :], start=(k == 0), stop=(k == KC - 1))
                    nc.vector.tensor_tensor(
                        vloc[:, t, :, 0:64],
                        ps[:].rearrange("p (h d) -> p h d", h=H),
                        vb_bc[:].rearrange("p (h d) -> p h d", h=H), ALU.add)
                vlocs[s] = vloc
                # rope: qkv1 (de-interleaved) -> qkr (natural order), DVE fp16
                qkr = a8.tile([128, 8, S], F16, tag="a8")
                for half in (0, 4):
                    for c in range(2):
                        e = qkv1[:, half + c, :]
                        o = qkv1[:, half + 2 + c, :]
                        r1 = qkr[:, half + c, :]
                        r2 = qkr[:, half + 2 + c, :]
                        t1 = rtp.tile([128, S], F16, tag="rt")
                        nc.vector.tensor_tensor(r1, e, gridc[:, c, :], ALU.mult)
                        nc.vector.tensor_tensor(t1[:], o, grids[:, c, :], ALU.mult)
                        nc.vector.tensor_tensor(r1, r1, t1[:], ALU.subtract)
                        t2 = rtp.tile([128, S], F16, tag="rt")
                        nc.vector.tensor_tensor(r2, e, grids[:, c, :], ALU.mult)
                        nc.vector.tensor_tensor(t2[:], o, gridc[:, c, :], ALU.mult)
                        nc.vector.tensor_tensor(r2, r2, t2[:], ALU.add)
                qkrs[s] = qkr

            # ---- phase A2 per sample: stage2, attention, out-proj ----
            rc2, h2s = {}, {}
            for s in range(BL):
                sl = slice(s * S, (s + 1) * S)
                qkr, vloc = qkrs[s], vlocs[s]
                qk2 = qk2p.tile([128, 8, S], F16, tag="qk2")
                for m in range(8):
                    ps = b1.tile([128, S], F32, tag="b1")
                    base = 0 if m < 4 else 4
                    for k in range(KC):
                        nc.tensor.matmul(ps[:], w2[:, k, m * 128:(m + 1) * 128],
                                         qkr[:, base + k, :],
                                         start=(k == 0), stop=(k == KC - 1))
                    nc.vector.tensor_tensor(qk2[:, m, :], ps[:], cq[:, m, :],
                                            ALU.add)
                # attention heads, software-pipelined one pair ahead:
                # scores+exp of pair k+1 are emitted before A@V of pair k, so
                # the PE fills pair k's exp latency with pair k+1's scores.
                o_t = a4.tile([128, KC, S], F16, tag="ot", bufs=2)
                ats = {}

                def emit_scores(pair):
                    for hh in pair:
                        rows = slice(64 * (hh % 2), 64 * (hh % 2) + 64)
                        at = a4.tile([128, KC, S], F16, tag="at", bufs=4,
                                     name=f"at{hh}")
                        for c in range(KC):
                            scp = b1.tile([128, S], F32, tag="b1")
                            nc.tensor.matmul(
                                scp[:],
                                qk2[rows, 4 + hh // 2, c * 128:(c + 1) * 128],
                                qk2[rows, hh // 2, :], start=True, stop=True)
                            nc.scalar.activation(at[:, c, :], scp[:], AF.Exp,
                                                 scale=float(1.0 / np.sqrt(DH)))
                        ats[hh] = at

                pairs = [(0, 1), (2, 3), (4, 5), (6, 7)]
                emit_scores(pairs[0])
                for pi, pair in enumerate(pairs):
                    if pi + 1 < len(pairs):
                        emit_scores(pairs[pi + 1])
                    for hh in pair:
                        rows = slice(64 * (hh % 2), 64 * (hh % 2) + 64)
                        at = ats[hh]
                        ov = b2.tile([65, S], F32, tag="b2")
                        for c in range(KC):
                            nc.tensor.matmul(ov[:], vloc[:, c, hh, :], at[:, c, :],
                                             start=(c == 0), stop=(c == KC - 1))
                        # 1/denom = exp(-ln denom), broadcast via PE
                        lnd = scr.tile([1, S], F16, tag="lnd")
                        nc.scalar.activation(lnd[:], ov[64:65, :], AF.Ln)
                        rb = b2.tile([64, S], F32, tag="b2")
                        nc.tensor.matmul(rb[:], ones_row[:, 0:64], lnd[:],
                                         start=True, stop=True)
                        inv = invp.tile([64, S], F16, tag="inv")
                        nc.scalar.activation(inv[:], rb[:], AF.Exp, scale=-1.0)
                        nc.vector.tensor_tensor(o_t[rows, hh // 2, :],
                                                ov[0:64, :], inv[:], ALU.mult)
                # out-proj + residual
                for m in range(KC):
                    ps = b1.tile([128, S], F32, tag="b1")
                    for k in range(KC):
                        nc.tensor.matmul(ps[:], ow[:, k, m * 128:(m + 1) * 128],
                                         o_t[:, k, :], start=(k == 0),
                                         stop=(k == KC - 1))
                    nc.vector.scalar_tensor_tensor(
                        x[:, m, sl], ps[:], owb[:, m:m + 1], x[:, m, sl],
                        ALU.add, ALU.add)
                rc2[s] = ln_stats(x, s)
                h2s[s] = hp.tile([128, KC, S], F8, tag="h8", name=f"h2_{s}")
                ln_apply(rc2[s], x, s, h2s[s], scaled=True)

            # ---- phase B: FFN in fp8/DoubleRow (l2 streams in once w1 slot
            #      is released); biases ride in the PSUM via a rank-1 matmul.
            l2 = wpool.tile([128, FC, D], F8, tag="bigw")
            for kf in range(FC):
                nc.sync.dma_start(l2[:, kf, :], dram["el2T"][li][:, kf, :])
            for s in range(BL):
                sl = slice(s * S, (s + 1) * S)
                h2 = h2s[s]                               # h2 = HS*LN(x)
                f2 = [b1.tile([128, S], F32, tag="b1", name=f"f2_{_m}")
                      for _m in range(KC)]
                for m in range(KC):     # inject WOS*l2b into the accumulators
                    nc.tensor.matmul(f2[m][:], l2b[:, m * 128:(m + 1) * 128],
                                     ones_rowS[:], start=True, stop=False)
                for jf in range(FC // 2):
                    rt2 = rtp.tile([128, 2, S], F8, tag="rt8")
                    for i in range(2):
                        kf = 2 * jf + i
                        f1 = b2.tile([128, S], F32, tag="b2")
                        for c in range(2):
                            nc.tensor.matmul(
                                f1[:],
                                l1[:, 2 * c:2 * c + 2, kf * 128:(kf + 1) * 128],
                                h2[:, 2 * c:2 * c + 2, :],
                                start=(c == 0), stop=(c == 1), perf_mode=DR)
                        # rt = OS*relu(z + l1b):  f1 = WHS*z, bias = OS*l1b
                        nc.scalar.activation(rt2[:, i, :], f1[:], AF.Relu,
                                             scale=OS / WHS,
                                             bias=l1b[:, kf:kf + 1])
                    for m in range(KC):
                        nc.tensor.matmul(f2[m][:],
                                         l2[:, 2 * jf:2 * jf + 2, m * 128:(m + 1) * 128],
                                         rt2[:, :, :], start=False,
                                         stop=(jf == FC // 2 - 1), perf_mode=DR)
                for m in range(KC):
                    nc.vector.scalar_tensor_tensor(
                        x[:, m, sl], f2[m][:], 1.0 / WOS, x[:, m, sl],
                        ALU.mult, ALU.add)
                rc1[s] = ln_stats(x, s)   # next layer (or the final LN)
                if li + 1 < le:           # prefetch next layer's LN1 apply
                    hs_next[s] = hp.tile([128, KC, S], F16, tag="h",
                                         name=f"h_pre{s}")
                    ln_apply(rc1[s], x, s, hs_next[s])

        # ---------------- final encoder LN (in-place; affine folded) --------
        me = x
        for s in range(BL):
            ln_apply(rc1[s], x, s, x[:, :, s * S:(s + 1) * S])

        # ---------------- decoder layers ----------------
        for li in range(ld):
            dw = wpool.tile([128, KC, 3 * D], F16, tag="bigw")
            nc.sync.dma_start(dw[:], dram["dinT"][li])
            dwb = bpool.tile([128, 12], F32, tag="w1b")
            nc.sync.dma_start(dwb[:], dram["dinb"][li])
            dvbrow = bpool.tile([1, D], F16, tag="vbrow")
            nc.sync.dma_start(dvbrow[:], dram["dvb"][li])
            do = owpool.tile([128, KC, D], F16, tag="ow")
            nc.sync.dma_start(do[:], dram["dowT"][li])
            dob = bpool.tile([128, 4], F32, tag="owb")
            nc.sync.dma_start(dob[:], dram["dowb"][li])
            m1 = wpool.tile([128, KC, M], F16, tag="bigw")
            nc.sync.dma_start(m1[:], dram["dm1T"][li])
            m1b = bpool.tile([128, 16], F32, tag="l1b")
            nc.sync.dma_start(m1b[:], dram["dm1b"][li])
            m2b = bpool.tile([128, 4], F32, tag="l2b")
            nc.sync.dma_start(m2b[:], dram["dm2b"][li])

            vb_ps = b1.tile([128, D], F32, tag="b1")
            nc.tensor.matmul(vb_ps[:], ones_row[:], dvbrow[:], start=True, stop=True)
            vb_bc = vbp.tile([128, D], F16, tag="vb_bc")
            nc.scalar.activation(vb_bc[:], vb_ps[:], AF.Copy)

            # LN(p) -> q_ln ; Q projection (all samples at once, N=BL)
            q_ln = smalls.tile([128, KC, BL], F16, tag="q_ln")
            ln_small(p, BL, q_ln)
            qps = b1.tile([128, KC, BL], F32, tag="b1")
            for m in range(KC):
                for k in range(KC):
                    nc.tensor.matmul(qps[:, m, :],
                                     dw[:, k, m * 128:(m + 1) * 128],
                                     q_ln[:, k, :], start=(k == 0),
                                     stop=(k == KC - 1))
            q_sb = smalls.tile([128, KC, BL], F16, tag="q_sb")
            for m in range(KC):
                nc.scalar.activation(q_sb[:, m, :], qps[:, m, :], AF.Identity,
                                     bias=dwb[:, m:m + 1])
            o_d = smalls.tile([128, KC, BL], F16, tag="o_d")
            for s in range(BL):
                sl = slice(s * S, (s + 1) * S)
                # K (feature-major) and V' (token-major) over morph_enc
                k_sb = a4.tile([128, KC, S], F16, tag="at", bufs=4)
                for m in range(KC):
                    ps = b1.tile([128, S], F32, tag="b1")
                    for k in range(KC):
                        nc.tensor.matmul(
                            ps[:], dw[:, k, D + m * 128:D + (m + 1) * 128],
                            me[:, k, sl], start=(k == 0), stop=(k == KC - 1))
                    nc.scalar.activation(k_sb[:, m, :], ps[:], AF.Identity,
                                         bias=dwb[:, 4 + m:5 + m])
                vloc = vp.tile([128, KC, 8, 65], F16, tag="vloc")
                for t in range(KC):
                    nc.vector.tensor_copy(vloc[:, t, :, 64], ones8[:])
                for t in range(KC):
                    ps = b1.tile([128, S], F32, tag="b1")
                    for k in range(KC):
                        nc.tensor.matmul(
                            ps[:], me[:, k, s * S + t * 128:s * S + (t + 1) * 128],
                            dw[:, k, 2 * D:3 * D],
                            start=(k == 0), stop=(k == KC - 1))
                    nc.vector.tensor_tensor(
                        vloc[:, t, :, 0:64],
                        ps[:].rearrange("p (h d) -> p h d", h=H),
                        vb_bc[:].rearrange("p (h d) -> p h d", h=H), ALU.add)
                scp = b1.tile([128, KC, H], F32, tag="b1")
                for hh in range(H):
                    rows = slice(64 * (hh % 2), 64 * (hh % 2) + 64)
                    for c in range(KC):
                        nc.tensor.matmul(
                            scp[:, c, hh:hh + 1],
                            k_sb[rows, hh // 2, c * 128:(c + 1) * 128],
                            q_sb[rows, hh // 2, s:s + 1],
                            start=True, stop=True)
                at = smalls.tile([128, KC, H], F16, tag="at_d")
                nc.scalar.activation(at[:], scp[:], AF.Exp,
                                     scale=float(1.0 / np.sqrt(DH)))
                ov = b2.tile([65, H], F32, tag="b2")
                for hh in range(H):
                    for c in range(KC):
                        nc.tensor.matmul(ov[:, hh:hh + 1], vloc[:, c, hh, :],
                                         at[:, c, hh:hh + 1],
                                         start=(c == 0), stop=(c == KC - 1))
                # 1/denom = exp(-ln denom)
                lnd = scr.tile([1, H], F16, tag="lnd")
                nc.scalar.activation(lnd[:], ov[64:65, :], AF.Ln)
                rb = b2.tile([64, H], F32, tag="b2")
                nc.tensor.matmul(rb[:], ones_row[:, 0:64], lnd[:],
                                 start=True, stop=True)
                inv = invp.tile([64, H], F16, tag="inv_d")
                nc.scalar.activation(inv[:], rb[:], AF.Exp, scale=-1.0)
                for hh in range(H):
                    rows = slice(64 * (hh % 2), 64 * (hh % 2) + 64)
                    nc.vector.tensor_tensor(o_d[rows, hh // 2, s:s + 1],
                                            ov[0:64, hh:hh + 1],
                                            inv[:, hh:hh + 1], ALU.mult)
            # out-proj + residual into p
            ops = b1.tile([128, KC, BL], F32, tag="b1")
            for m in range(KC):
                for k in range(KC):
                    nc.tensor.matmul(ops[:, m, :],
                                     do[:, k, m * 128:(m + 1) * 128],
                                     o_d[:, k, :], start=(k == 0),
                                     stop=(k == KC - 1))
            for m in range(KC):
                nc.vector.scalar_tensor_tensor(
                    p[:, m, :], ops[:, m, :], dob[:, m:m + 1], p[:, m, :],
                    ALU.add, ALU.add)
            # FFN on p (m2 streams in chunked once dw releases its slot)
            m2 = wpool.tile([128, MC, D], F16, tag="bigw")
            for kf in range(MC):
                nc.sync.dma_start(m2[:, kf, :], dram["dm2T"][li][:, kf, :])
            h2d = smalls.tile([128, KC, BL], F16, tag="q_ln")
            ln_small(p, BL, h2d)
            mh = smalls.tile([128, MC, BL], F16, tag="mh")
            for mm_ in range(MC):
                ps = b1.tile([128, BL], F32, tag="b1")
                for k in range(KC):
                    nc.tensor.matmul(ps[:], m1[:, k, mm_ * 128:(mm_ + 1) * 128],
                                     h2d[:, k, :], start=(k == 0),
                                     stop=(k == KC - 1))
                nc.scalar.activation(mh[:, mm_, :], ps[:], AF.Relu,
                                     bias=m1b[:, mm_:mm_ + 1])
            m2ps = b1.tile([128, KC, BL], F32, tag="b1")
            for m in range(KC):
                for kf in range(MC):
                    nc.tensor.matmul(m2ps[:, m, :],
                                     m2[:, kf, m * 128:(m + 1) * 128],
                                     mh[:, kf, :], start=(kf == 0),
                                     stop=(kf == MC - 1))
            for m in range(KC):
                nc.vector.scalar_tensor_tensor(
                    p[:, m, :], m2ps[:, m, :], m2b[:, m:m + 1], p[:, m, :],
                    ALU.add, ALU.add)

        # ---------------- head ----------------
        hw = smalls.tile([128, KC], F16, tag="hw")
        hb = smalls.tile([1, 1], F32, tag="hb")
        nc.sync.dma_start(hw[:], dram["hwT"][:])
        nc.sync.dma_start(hb[:], dram["hb"][:])
        hg = smalls.tile([128, KC, BL], F16, tag="q_ln")
        ln_small(p, BL, hg)
        hps = b2.tile([1, BL], F32, tag="b2")
        for k in range(KC):
            nc.tensor.matmul(hps[:], hw[:, k:k + 1], hg[:, k, :],
                             start=(k == 0), stop=(k == KC - 1))
        y_sb = smalls.tile([1, BL], F32, tag="y_sb")
        nc.scalar.activation(y_sb[:], hps[:], AF.Sigmoid, bias=hb[:])
        nc.sync.dma_start(y_dram[:], y_sb[:])


# ----------------------------------------------------------------------------
# entry point
# ----------------------------------------------------------------------------

_NC_CACHE = {}


def kernel(**inputs):
    return _run(inputs, LE, LD)


def _run(inputs, le, ld, trace=False):
    w = prep_weights(inputs, le, ld)
    morph = np.asarray(inputs["morph"], np.float32)
    pose = np.asarray(inputs["pose"], np.float32)
    in_maps = []
    for c in range(NCORES):
        im = dict(w)
        mo = morph[c * BL:(c + 1) * BL]                 # [BL, S, 3]
        im["morphT"] = np.ascontiguousarray(
            mo.transpose(2, 0, 1).reshape(3, T)).astype(np.float16)
        im["poseT"] = np.ascontiguousarray(
            pose[c * BL:(c + 1) * BL].T).astype(np.float16)
        in_maps.append(im)

    if ("nc", le, ld) not in _NC_CACHE:
        _NC_CACHE[("nc", le, ld)] = build(le, ld)
    nc = _NC_CACHE[("nc", le, ld)]
    res = run_bass_kernel_spmd(nc, in_maps, core_ids=list(range(NCORES)),
                               trace=trace)
    out = np.zeros((B, 1), np.float32)
    for c in range(NCORES):
        out[c * BL:(c + 1) * BL, 0] = res.results[c]["y"][0]
    if trace:
        return out, res
    return out


# revision 71
# speedup vs baseline: 1.9355x; 1.0300x over previous
"""Trainium2 Bass kernel for nn_ReachabilityClassifierTransformer.

Data-parallel over batch: 16 samples / 8 cores = 2 samples per core.
Each core runs the full network (6-layer encoder + 4-layer decoder + head)
on its 2 samples. No collectives.

Pipeline (evolved from an fp32r baseline at 2.36 ms to ~1.26 ms):
  - fp16 matmul operands everywhere except the FFN (stationary weights get
    Fast-Weight-Load; DVE elementwise ops run in 2x mode; DMA halved).
    PSUM stays fp32.
  - Encoder FFN runs in fp8e4 with DoubleRow perf mode (2 contraction
    chunks per instruction).  Power-of-2 scales (weights x64, LN output
    x16, hidden x32) are folded into the LN broadcast row, the Relu
    activation scale, and the residual descale; l2's bias is injected into
    the PSUM accumulation by a rank-1 ones matmul so the residual stays a
    single DVE pass.
  - V path folded on host: V = h @ (Wv2 Wv1)^T + bv  (no rope between the
    two V projections, so the double-projection quirk collapses).
  - Stage-1 q/k biases folded through rope into per-position bias tensors
    C_q/C_k = W2 @ rope(b1) + b2, added at the stage-2 PSUM copy (rope is
    linear, rotation depends only on position).
  - No Sqrt / no DVE reciprocal anywhere: LN rsqrt = exp(-0.5 ln(v+eps)),
    softmax 1/denom = exp(-ln denom) broadcast via PE.  ln/exp/copy/relu/
    square all live in one activation table set (forced via
    _patch_act_tables) -> no table-set reloads.
  - LN stats AND the LN apply for the next phase are emitted at the end of
    the previous phase, so their scalar chains hide under matmuls; the
    attention loop is software-pipelined one head-pair ahead so each
    pair's exp latency is covered by the next pair's score matmuls.

Device layout conventions (per core):
  - Activations FEATURE-MAJOR in SBUF: tile [128, KC, T] holds X.T.
  - Weights pre-transposed on host to [in_feat, out_feat], laid out
    [128, KC_in, O] (partition = in-feature % 128).
  - matmul(out_psum[M,N], lhsT=[K,M], rhs=[K,N]) computes lhsT.T @ rhs.
  - Encoder stage-1 q,k output features are de-interleaved (even feats then
    odd feats) via host-side column permutation, so RoPE becomes contiguous
    block ops; the roped result is in natural (concatenated) order.
  - Softmax: scores computed transposed (S.T = K_h @ Q_h.T per chunk),
    exp'd without max subtraction (|scores/8| < 1 for this model), and the
    denominator comes free from a ones-column appended to V.
"""
import functools

import numpy as np

import concourse.bass as bass
import concourse.mybir as mybir
import concourse.tile as tile
from concourse import bacc
from concourse.bass_utils import run_bass_kernel_spmd


def _patch_act_tables():
    """Constrain exp/ln to the one table set that contains both.

    The act-table-load pass maps each activation function to a set
    independently (exp -> exp_and_others, ln -> natural_log), so a kernel
    that interleaves exp and ln reloads tables on every transition
    (~1.3us each).  natural_log_exp_and_others contains exp AND ln (plus
    copy/identity/relu/square), so restricting exp/ln to that set makes
    every load resolve there; set ids/order are preserved so the emitted
    act_func_set_id still indexes the real act_info.json.
    """
    import concourse.hw_specs as hw_specs
    if getattr(hw_specs, "_ant_act_tables_patched", False):
        return
    orig = hw_specs.get_activation_tables

    @functools.cache
    def patched(module_arch):
        t = orig(module_arch)
        keep = "natural_log_exp_and_others"
        if keep not in t:
            return t
        drop = {mybir.ActivationFunctionType.Exp, mybir.ActivationFunctionType.Ln}
        return {name: (fns if name == keep else fns - drop)
                for name, fns in t.items()}

    hw_specs._ant_act_tables_patched = True
    hw_specs.get_activation_tables = patched
    import sys
    for modname in ("concourse.bacc", "concourse.bass_interp"):
        mod = sys.modules.get(modname)
        if mod is not None and hasattr(mod, "get_activation_tables"):
            mod.get_activation_tables = patched


_patch_act_tables()

AF = mybir.ActivationFunctionType
ALU = mybir.AluOpType
F32 = mybir.dt.float32
F16 = mybir.dt.float16
F8 = mybir.dt.float8e4
F32R = mybir.dt.float32r
F8NP = mybir.dt.np(F8)
DR = mybir.MatmulPerfMode.DoubleRow
F8MAX = 240.0          # TRN fp8e4 saturation (not OCP's 448)
WS = 64.0              # fp8 FFN weight scale
HS = 16.0              # fp8 FFN input activation scale
OS = 32.0              # fp8 FFN hidden scale
WHS = WS * HS          # l1 psum descale
WOS = WS * OS          # l2 psum descale

B, S, D, FF, H, LE, LD, M = 16, 512, 512, 2048, 8, 6, 4, 2048
ROPE_BASE = 10000.0
LN_EPS = 1e-5
NCORES = 8
BL = B // NCORES          # 2 samples per core
T = BL * S                # 1024 tokens per core
KC = D // 128             # 4 feature chunks
FC = FF // 128            # 16
MC = M // 128             # 16
DH = D // H               # 64


# ----------------------------------------------------------------------------
# host-side helpers
# ----------------------------------------------------------------------------

def _chunked(wT):
    """[Din, O] -> [128, Din//128, O] contiguous fp16."""
    Din, O = wT.shape
    return np.ascontiguousarray(
        wT.reshape(Din // 128, 128, O).transpose(1, 0, 2)).astype(np.float16)


def _bias_cols(b):
    """[O] -> [128, O//128]  (column per 128-chunk), fp32."""
    O = b.shape[0]
    return np.ascontiguousarray(b.reshape(O // 128, 128).T).astype(np.float32)


def _chunked8(wT, scale=WS):
    """[Din, O] -> [128, Din//128, O] contiguous fp8e4, pre-scaled."""
    Din, O = wT.shape
    a = np.clip(wT * scale, -F8MAX, F8MAX)
    return np.ascontiguousarray(
        a.reshape(Din // 128, 128, O).transpose(1, 0, 2)).astype(F8NP)


_DEINT = np.concatenate([np.arange(0, D, 2), np.arange(1, D, 2)])  # de-interleave


def prep_weights(inp, le=LE, ld=LD):
    """Host-side weight prep -> dict of arrays shared by all cores."""
    out = {}
    g = {k: np.asarray(v, np.float64) for k, v in inp.items()}

    out["mpwT"] = np.ascontiguousarray(g["morph_proj_w"].T).astype(np.float16)
    out["mpb"] = _bias_cols(g["morph_proj_b"])                     # [128, 4]
    out["ppwT"] = np.ascontiguousarray(g["pose_proj_w"].T).astype(np.float16)
    out["ppb"] = _bias_cols(g["pose_proj_b"])

    # rope grids, de-interleaved frequency order: [128, 2, 512] fp16
    freq = 1.0 / ROPE_BASE ** (np.arange(0, D, 2, dtype=np.float64) / D)
    ang = np.outer(np.arange(S, dtype=np.float64), freq)           # [512, 256]
    cosT = np.cos(ang).T                                           # [256, S]
    sinT = np.sin(ang).T
    out["gridc"] = _chunked(cosT.reshape(256, S))
    out["grids"] = _chunked(sinT.reshape(256, S))

    e_w1, e_w2, e_wv, e_cqk, e_vb = [], [], [], [], []
    e_ow, e_owb, e_l1, e_l1b, e_l2, e_l2b = [], [], [], [], [], []
    for i in range(le):
        w1 = g["enc_in_w"][i] * g["enc_n1_g"][i][None, :]          # fold n1 g
        b1 = g["enc_in_b"][i] + g["enc_in_w"][i] @ g["enc_n1_b"][i]
        # stage-1 q,k only, de-interleaved output columns
        perm = np.concatenate([_DEINT, D + _DEINT])
        e_w1.append(_chunked(np.ascontiguousarray(w1[perm].T)))    # [128,4,1024]
        # stage-2 q,k (natural order, raw weights - the faithful quirk)
        w2 = g["enc_in_w"][i][: 2 * D]                             # Wq;Wk
        e_w2.append(_chunked(np.ascontiguousarray(w2.T)))          # [128,4,1024]
        # stage-2 bias tensors: C = W2 @ rope(b1) + b2   [512, S] each
        bq = b1[:D][_DEINT]                                        # [even; odd]
        bk = b1[D:2 * D][_DEINT]
        rb_q = np.concatenate([bq[:256, None] * cosT - bq[256:, None] * sinT,
                               bq[:256, None] * sinT + bq[256:, None] * cosT])
        rb_k = np.concatenate([bk[:256, None] * cosT - bk[256:, None] * sinT,
                               bk[:256, None] * sinT + bk[256:, None] * cosT])
        Cq = g["enc_in_w"][i][:D] @ rb_q + g["enc_in_b"][i][:D][:, None]
        Ck = g["enc_in_w"][i][D:2 * D] @ rb_k \
            + g["enc_in_b"][i][D:2 * D][:, None]
        C = np.concatenate([Cq, Ck], axis=0)                       # [1024, S]
        e_cqk.append(_chunked(C))                                  # [128,8,S]
        # V folded: V = h @ (Wv2 Wv1_f).T + (Wv2 bv1_f + bv2)
        Wv1f = w1[2 * D:]
        bv1f = b1[2 * D:]
        Wv2 = g["enc_in_w"][i][2 * D:]
        bv2 = g["enc_in_b"][i][2 * D:]
        e_wv.append(_chunked(np.ascontiguousarray((Wv2 @ Wv1f).T)))
        e_vb.append((Wv2 @ bv1f + bv2)[None, :].astype(np.float16))  # [1,512]
        e_ow.append(_chunked(np.ascontiguousarray(g["enc_out_w"][i].T)))
        e_owb.append(_bias_cols(g["enc_out_b"][i]))
        l1 = g["enc_l1_w"][i] * g["enc_n2_g"][i][None, :]
        l1b = g["enc_l1_b"][i] + g["enc_l1_w"][i] @ g["enc_n2_b"][i]
        e_l1.append(_chunked8(np.ascontiguousarray(l1.T)))         # [128,4,2048]
        e_l1b.append(_bias_cols(OS * l1b))                         # [128,16]
        e_l2.append(_chunked8(np.ascontiguousarray(g["enc_l2_w"][i].T)))
        e_l2b.append((WOS * g["enc_l2_b"][i])[None, :].astype(np.float16))
    out["ew1T"] = np.stack(e_w1) if le else np.zeros((0, 128, KC, 2 * D), np.float16)
    out["ew2T"] = np.stack(e_w2) if le else np.zeros((0, 128, KC, 2 * D), np.float16)
    out["ewvT"] = np.stack(e_wv) if le else np.zeros((0, 128, KC, D), np.float16)
    out["ecqk"] = np.stack(e_cqk) if le else np.zeros((0, 128, 8, S), np.float16)
    out["evb"] = np.stack(e_vb) if le else np.zeros((0, 1, D), np.float16)
    out["eowT"] = np.stack(e_ow) if le else np.zeros((0, 128, KC, D), np.float16)
    out["eowb"] = np.stack(e_owb) if le else np.zeros((0, 128, 4), np.float32)
    out["el1T"] = np.stack(e_l1) if le else np.zeros((0, 128, KC, FF), F8NP)
    out["el1b"] = np.stack(e_l1b) if le else np.zeros((0, 128, 16), np.float32)
    out["el2T"] = np.stack(e_l2) if le else np.zeros((0, 128, FC, D), F8NP)
    out["el2b"] = np.stack(e_l2b) if le else np.zeros((0, 1, D), np.float16)

    d_in, d_inb, d_vb, d_ow, d_owb = [], [], [], [], []
    d_m1, d_m1b, d_m2, d_m2b = [], [], [], []
    for i in range(ld):
        w = g["dec_in_w"][i].copy()
        b = g["dec_in_b"][i].copy()
        w[:D] = w[:D] * g["dec_n1_g"][i][None, :]                  # Wq <- dec_n1
        b[:D] = b[:D] + g["dec_in_w"][i][:D] @ g["dec_n1_b"][i]
        w[D:] = w[D:] * g["enc_final_g"][None, :]                  # Wk,Wv <- enc_final
        b[D:] = b[D:] + g["dec_in_w"][i][D:] @ g["enc_final_b"]
        d_in.append(_chunked(np.ascontiguousarray(w.T)))           # [128,4,1536]
        d_inb.append(_bias_cols(b))
        d_vb.append(b[2 * D:][None, :].astype(np.float16))         # [1,512]
        d_ow.append(_chunked(np.ascontiguousarray(g["dec_out_w"][i].T)))
        d_owb.append(_bias_cols(g["dec_out_b"][i]))
        m1 = g["dec_m1_w"][i] * g["dec_n2_g"][i][None, :]
        m1b = g["dec_m1_b"][i] + g["dec_m1_w"][i] @ g["dec_n2_b"][i]
        d_m1.append(_chunked(np.ascontiguousarray(m1.T)))          # [128,4,2048]
        d_m1b.append(_bias_cols(m1b))
        d_m2.append(_chunked(np.ascontiguousarray(g["dec_m2_w"][i].T)))
        d_m2b.append(_bias_cols(g["dec_m2_b"][i]))
    out["dinT"] = np.stack(d_in) if ld else np.zeros((0, 128, KC, 3 * D), np.float16)
    out["dinb"] = np.stack(d_inb) if ld else np.zeros((0, 128, 12), np.float32)
    out["dvb"] = np.stack(d_vb) if ld else np.zeros((0, 1, D), np.float16)
    out["dowT"] = np.stack(d_ow) if ld else np.zeros((0, 128, KC, D), np.float16)
    out["dowb"] = np.stack(d_owb) if ld else np.zeros((0, 128, 4), np.float32)
    out["dm1T"] = np.stack(d_m1) if ld else np.zeros((0, 128, KC, M), np.float16)
    out["dm1b"] = np.stack(d_m1b) if ld else np.zeros((0, 128, 16), np.float32)
    out["dm2T"] = np.stack(d_m2) if ld else np.zeros((0, 128, MC, D), np.float16)
    out["dm2b"] = np.stack(d_m2b) if ld else np.zeros((0, 128, 4), np.float32)

    hw = (g["head_w"] * g["head_g"][None, :])[0]                   # [512]
    out["hwT"] = _bias_cols(hw).astype(np.float16)                 # [128, 4]
    out["hb"] = (g["head_bias"] + g["head_w"] @ g["head_b"]).reshape(1, 1).astype(np.float32)
    return out


# ----------------------------------------------------------------------------
# device program
# ----------------------------------------------------------------------------

def build(le=LE, ld=LD):
    nc = bacc.Bacc(None, target_bir_lowering=False)

    dram = {}

    def din(name, shape, dt=F16):
        dram[name] = nc.dram_tensor(name, list(shape), dt, kind="ExternalInput")
        return dram[name]

    # shared weights
    din("mpwT", [3, 512]); din("mpb", [128, 4], F32)
    din("ppwT", [9, 512]); din("ppb", [128, 4], F32)
    din("gridc", [128, 2, S]); din("grids", [128, 2, S])
    din("ew1T", [le, 128, KC, 2 * D])
    din("ew2T", [le, 128, KC, 2 * D])
    din("ewvT", [le, 128, KC, D])
    din("ecqk", [le, 128, 8, S])
    din("evb", [le, 1, D])
    din("eowT", [le, 128, KC, D]); din("eowb", [le, 128, 4], F32)
    din("el1T", [le, 128, KC, FF], F8); din("el1b", [le, 128, 16], F32)
    din("el2T", [le, 128, FC, D], F8); din("el2b", [le, 1, D])
    din("dinT", [ld, 128, KC, 3 * D]); din("dinb", [ld, 128, 12], F32)
    din("dvb", [ld, 1, D])
    din("dowT", [ld, 128, KC, D]); din("dowb", [ld, 128, 4], F32)
    din("dm1T", [ld, 128, KC, M]); din("dm1b", [ld, 128, 16], F32)
    din("dm2T", [ld, 128, MC, D]); din("dm2b", [ld, 128, 4], F32)
    din("hwT", [128, KC]); din("hb", [1, 1], F32)
    # per-core inputs
    din("morphT", [3, T])
    din("poseT", [9, BL])
    y = nc.dram_tensor("y", [1, BL], F32, kind="ExternalOutput")

    with tile.TileContext(nc) as tc:
        _build_body(nc, tc, dram, y, le, ld)
    nc.compile()
    return nc


def _build_body(nc, tc, dram, y_dram, le, ld):
    import contextlib
    ctx = contextlib.ExitStack()
    with ctx:
        ctx.enter_context(nc.allow_low_precision(
            reason="fp16 matmul operands / activations are intentional"))
        persist = ctx.enter_context(tc.tile_pool(name="persist", bufs=1))
        wpool = ctx.enter_context(tc.tile_pool(name="wpool", bufs=3))
        w2pool = ctx.enter_context(tc.tile_pool(name="w2pool", bufs=1))
        owpool = ctx.enter_context(tc.tile_pool(name="owpool", bufs=1))
        vwpool = ctx.enter_context(tc.tile_pool(name="vwpool", bufs=1))
        cqpool = ctx.enter_context(tc.tile_pool(name="cqpool", bufs=1))
        bpool = ctx.enter_context(tc.tile_pool(name="bpool", bufs=2))
        a4 = ctx.enter_context(tc.tile_pool(name="a4", bufs=4))
        hp = ctx.enter_context(tc.tile_pool(name="hp", bufs=2))
        a8 = ctx.enter_context(tc.tile_pool(name="a8", bufs=3))
        qk2p = ctx.enter_context(tc.tile_pool(name="qk2p", bufs=1))
        vp = ctx.enter_context(tc.tile_pool(name="vp", bufs=2))
        rtp = ctx.enter_context(tc.tile_pool(name="rtp", bufs=3))
        vbp = ctx.enter_context(tc.tile_pool(name="vbp", bufs=1))
        scr = ctx.enter_context(tc.tile_pool(name="scr", bufs=3))
        invp = ctx.enter_context(tc.tile_pool(name="invp", bufs=2))
        rcp = ctx.enter_context(tc.tile_pool(name="rcp", bufs=2))
        smalls = ctx.enter_context(tc.tile_pool(name="smalls", bufs=2))
        b1 = ctx.enter_context(tc.tile_pool(name="b1", bufs=4, space="PSUM"))
        b2 = ctx.enter_context(tc.tile_pool(name="b2", bufs=4, space="PSUM"))

        # ---------------- persistent tiles ----------------
        x = persist.tile([128, KC, T], F16)           # residual stream (X.T)
        gridc = persist.tile([128, 2, S], F16)
        grids = persist.tile([128, 2, S], F16)
        ones128 = persist.tile([128, 1], F16)
        ones_row = persist.tile([1, 128], F16)
        ones_rowS = persist.tile([1, S], F16)         # bias-inject moving row
        hs_row = persist.tile([1, 128], F16)          # HS-scaled broadcast row
        ones8 = persist.tile([128, 8], F16)
        eps_t = persist.tile([1, 1], F32)
        p = persist.tile([128, KC, BL], F16)          # decoder latent p.T
        nc.sync.dma_start(gridc[:], dram["gridc"][:])
        nc.sync.dma_start(grids[:], dram["grids"][:])
        stage_f16 = rtp.tile([128, 128], F16, tag="rt")
        nc.vector.memset(stage_f16[:], 1.0)
        nc.vector.tensor_copy(ones128[:], stage_f16[:, 0:1])
        nc.vector.tensor_copy(ones_row[:], stage_f16[0:1, :])
        nc.vector.tensor_copy(ones8[:], stage_f16[:, 0:8])
        nc.vector.memset(hs_row[:], HS)
        nc.vector.memset(ones_rowS[:], 1.0)
        nc.vector.memset(eps_t[:], LN_EPS)

        def ln_small(x_tile, n_tok, h_out):
            """h_out = LayerNorm_features(x_tile) for tiny n_tok (decoder)."""
            sq = smalls.tile([128, KC, n_tok], F16, tag="sq_d")
            for k in range(KC):
                nc.vector.tensor_tensor(sq[:, k, :], x_tile[:, k, :],
                                        x_tile[:, k, :], ALU.mult)
            sum_ps = b2.tile([1, n_tok], F32, tag="b2")
            sq_ps = b2.tile([1, n_tok], F32, tag="b2")
            for k in range(KC):
                nc.tensor.matmul(sum_ps[:], ones128[:], x_tile[:, k, :],
                                 start=(k == 0), stop=(k == KC - 1))
            for k in range(KC):
                nc.tensor.matmul(sq_ps[:], ones128[:], sq[:, k, :],
                                 start=(k == 0), stop=(k == KC - 1))
            ms = scr.tile([1, n_tok], F32, tag="scr")
            t2 = scr.tile([1, n_tok], F32, tag="scr")
            rc = scr.tile([1, 2, n_tok], F16, tag="scr_rc")
            nc.scalar.activation(ms[:], sum_ps[:], AF.Copy, scale=1.0 / D)
            nc.vector.tensor_tensor(t2[:], ms[:], ms[:], ALU.mult)      # m^2
            nc.vector.scalar_tensor_tensor(
                t2[:], sq_ps[:], 1.0 / D, t2[:], ALU.mult, ALU.subtract)
            # r = exp(-0.5 ln(var + eps))
            nc.scalar.activation(t2[:], t2[:], AF.Ln, bias=eps_t[:])
            nc.scalar.activation(rc[:, 0, :], t2[:], AF.Exp, scale=-0.5)
            nc.vector.scalar_tensor_tensor(
                rc[:, 1, :], ms[:], -1.0, rc[:, 0, :], ALU.mult, ALU.mult)
            r_ps = b1.tile([128, n_tok], F32, tag="b1")
            c_ps = b1.tile([128, n_tok], F32, tag="b1")
            nc.tensor.matmul(r_ps[:], ones_row[:], rc[:, 0, :],
                             start=True, stop=True)
            nc.tensor.matmul(c_ps[:], ones_row[:], rc[:, 1, :],
                             start=True, stop=True)
            for k in range(KC):
                nc.vector.tensor_tensor(h_out[:, k, :], x_tile[:, k, :],
                                        r_ps[:], ALU.mult)
                nc.vector.tensor_tensor(h_out[:, k, :], h_out[:, k, :],
                                        c_ps[:], ALU.add)

        def ln_stats(x_tile, s):
            """Per-sample LN stats -> rc [1, 2, S] fp16 (r, c)."""
            sl = slice(s * S, (s + 1) * S)
            sq = a4.tile([128, KC, S], F16, tag="sq", bufs=2)
            nc.vector.tensor_tensor(sq[:], x_tile[:, :, sl], x_tile[:, :, sl],
                                    ALU.mult)
            sum_ps = b2.tile([1, S], F32, tag="b2")
            sq_ps = b2.tile([1, S], F32, tag="b2")
            for k in range(KC):
                nc.tensor.matmul(sum_ps[:], ones128[:], x_tile[:, k, sl],
                                 start=(k == 0), stop=(k == KC - 1))
            for k in range(KC):
                nc.tensor.matmul(sq_ps[:], ones128[:], sq[:, k, :],
                                 start=(k == 0), stop=(k == KC - 1))
            ms = scr.tile([1, S], F32, tag="scr")
            t2 = scr.tile([1, S], F32, tag="scr")
            rc = rcp.tile([1, 2, S], F16, tag="rc", bufs=4)
            nc.scalar.activation(ms[:], sum_ps[:], AF.Copy, scale=1.0 / D)
            nc.vector.tensor_tensor(t2[:], ms[:], ms[:], ALU.mult)
            nc.vector.scalar_tensor_tensor(
                t2[:], sq_ps[:], 1.0 / D, t2[:], ALU.mult, ALU.subtract)
            nc.scalar.activation(t2[:], t2[:], AF.Ln, bias=eps_t[:])
            nc.scalar.activation(rc[:, 0, :], t2[:], AF.Exp, scale=-0.5)
            nc.vector.scalar_tensor_tensor(
                rc[:, 1, :], ms[:], -1.0, rc[:, 0, :], ALU.mult, ALU.mult)
            return rc

        def ln_apply(rc, x_tile, s, h_out, scaled=False):
            """h_out = (x[:, :, s] * r + c) * (HS if scaled else 1)."""
            sl = slice(s * S, (s + 1) * S)
            row = hs_row if scaled else ones_row
            r_ps = b1.tile([128, S], F32, tag="b1")
            c_ps = b1.tile([128, S], F32, tag="b1")
            nc.tensor.matmul(r_ps[:], row[:], rc[:, 0, :],
                             start=True, stop=True)
            nc.tensor.matmul(c_ps[:], row[:], rc[:, 1, :],
                             start=True, stop=True)
            rcb = rcp.tile([128, 2, S], F16, tag="rcb")
            nc.scalar.activation(rcb[:, 0, :], r_ps[:], AF.Copy)
            nc.scalar.activation(rcb[:, 1, :], c_ps[:], AF.Copy)
            for k in range(KC):
                tmp = rtp.tile([128, S], F16, tag="rt")
                nc.vector.tensor_tensor(tmp[:], x_tile[:, k, sl],
                                        rcb[:, 0, :], ALU.mult)
                nc.vector.tensor_tensor(h_out[:, k, :], tmp[:],
                                        rcb[:, 1, :], ALU.add)

        # ---------------- morph projection -> x ----------------
        morpht = rtp.tile([3, T], F16, tag="morph", bufs=1)
        nc.sync.dma_start(morpht[:], dram["morphT"][:])
        mpw = rtp.tile([3, 512], F16, tag="rt")
        mpb = rtp.tile([128, 4], F32, tag="rtb")
        nc.sync.dma_start(mpw[:], dram["mpwT"][:])
        nc.sync.dma_start(mpb[:], dram["mpb"][:])
        rc1, hs_next = {}, {}
        for s in range(BL):
            for m in range(KC):
                ps = b1.tile([128, S], F32, tag="b1")
                nc.tensor.matmul(ps[:], mpw[:, m * 128:(m + 1) * 128],
                                 morpht[:, s * S:(s + 1) * S], start=True, stop=True)
                nc.scalar.activation(x[:, m, s * S:(s + 1) * S], ps[:], AF.Relu,
                                     bias=mpb[:, m:m + 1])
            rc1[s] = ln_stats(x, s)
            hs_next[s] = hp.tile([128, KC, S], F16, tag="h", name=f"h_pre{s}")
            ln_apply(rc1[s], x, s, hs_next[s])

        # ---------------- pose projection -> p ----------------
        poset = rtp.tile([9, BL], F16, tag="rt")
        ppw = rtp.tile([9, 512], F16, tag="rt")
        ppb = rtp.tile([128, 4], F32, tag="rtb")
        nc.sync.dma_start(poset[:], dram["poseT"][:])
        nc.sync.dma_start(ppw[:], dram["ppwT"][:])
        nc.sync.dma_start(ppb[:], dram["ppb"][:])
        pps = b1.tile([128, KC, BL], F32, tag="b1")
        for m in range(KC):
            nc.tensor.matmul(pps[:, m, :], ppw[:, m * 128:(m + 1) * 128],
                             poset[:], start=True, stop=True)
        for m in range(KC):
            nc.scalar.activation(p[:, m, :], pps[:, m, :], AF.Relu,
                                 bias=ppb[:, m:m + 1])

        # ---------------- encoder layers ----------------
        for li in range(le):
            w1 = wpool.tile([128, KC, 2 * D], F16, tag="bigw")
            nc.sync.dma_start(w1[:], dram["ew1T"][li])
            w2 = w2pool.tile([128, KC, 2 * D], F16, tag="w2")
            nc.sync.dma_start(w2[:], dram["ew2T"][li])
            wv = vwpool.tile([128, KC, D], F16, tag="wv")
            nc.sync.dma_start(wv[:], dram["ewvT"][li])
            cq = cqpool.tile([128, 8, S], F16, tag="cq")
            nc.sync.dma_start(cq[:], dram["ecqk"][li])
            vbrow = bpool.tile([1, D], F16, tag="vbrow")
            nc.sync.dma_start(vbrow[:], dram["evb"][li])
            ow = owpool.tile([128, KC, D], F16, tag="ow")
            nc.sync.dma_start(ow[:], dram["eowT"][li])
            owb = bpool.tile([128, 4], F32, tag="owb")
            nc.sync.dma_start(owb[:], dram["eowb"][li])
            l1 = wpool.tile([128, KC, FF], F8, tag="bigw")
            nc.sync.dma_start(l1[:], dram["el1T"][li])
            l1b = bpool.tile([128, 16], F32, tag="l1b")
            nc.sync.dma_start(l1b[:], dram["el1b"][li])
            l2b = bpool.tile([1, D], F16, tag="l2br")
            nc.sync.dma_start(l2b[:], dram["el2b"][li])

            # v-bias broadcast [128, 512], once per layer
            vb_ps = b1.tile([128, D], F32, tag="b1")
            nc.tensor.matmul(vb_ps[:], ones_row[:], vbrow[:], start=True, stop=True)
            vb_bc = vbp.tile([128, D], F16, tag="vb_bc")
            nc.scalar.activation(vb_bc[:], vb_ps[:], AF.Copy)

            qkrs, vlocs = {}, {}
            # ---- phase A1 per sample: stage1 q,k; V; rope ----
            # (LN1 stats AND apply were emitted at the end of the previous
            #  phase so the whole chain overlapped that phase's matmuls)
            for s in range(BL):
                h = hs_next[s]
                qkv1 = a8.tile([128, 8, S], F16, tag="a8")
                for m in range(8):
                    ps = b1.tile([128, S], F32, tag="b1")
                    for k in range(KC):
                        nc.tensor.matmul(ps[:], w1[:, k, m * 128:(m + 1) * 128],
                                         h[:, k, :], start=(k == 0), stop=(k == KC - 1))
                    nc.scalar.activation(qkv1[:, m, :], ps[:], AF.Copy)
                # V token-major (+ones col), from h directly (folded weights)
                vloc = vp.tile([128, KC, 8, 65], F16, tag="vloc")
                for t in range(KC):
                    nc.vector.tensor_copy(vloc[:, t, :, 64], ones8[:])
                for t in range(KC):
                    ps = b1.tile([128, S], F32, tag="b1")
                    for k in range(KC):
                        nc.tensor.matmul(
                            ps[:], h[:, k, t * 128:(t + 1) * 128],
                            wv[:, k, :], start=(k == 0), stop=(k == KC - 1))
                    nc.vector.tensor_tensor(
                        vloc[:, t, :, 0:64],
                        ps[:].rearrange("p (h d) -> p h d", h=H),
                        vb_bc[:].rearrange("p (h d) -> p h d", h=H), ALU.add)
                vlocs[s] = vloc
                # rope: qkv1 (de-interleaved) -> qkr (natural order), DVE fp16
                qkr = a8.tile([128, 8, S], F16, tag="a8")
                for half in (0, 4):
                    for c in range(2):
                        e = qkv1[:, half + c, :]
                        o = qkv1[:, half + 2 + c, :]
                        r1 = qkr[:, half + c, :]
                        r2 = qkr[:, half + 2 + c, :]
                        t1 = rtp.tile([128, S], F16, tag="rt")
                        nc.vector.tensor_tensor(r1, e, gridc[:, c, :], ALU.mult)
                        nc.vector.tensor_tensor(t1[:], o, grids[:, c, :], ALU.mult)
                        nc.vector.tensor_tensor(r1, r1, t1[:], ALU.subtract)
                        t2 = rtp.tile([128, S], F16, tag="rt")
                        nc.vector.tensor_tensor(r2, e, grids[:, c, :], ALU.mult)
                        nc.vector.tensor_tensor(t2[:], o, gridc[:, c, :], ALU.mult)
                        nc.vector.tensor_tensor(r2, r2, t2[:], ALU.add)
                qkrs[s] = qkr

            # ---- phase A2 per sample: stage2, attention, out-proj ----
            rc2, h2s = {}, {}
            for s in range(BL):
                sl = slice(s * S, (s + 1) * S)
                qkr, vloc = qkrs[s], vlocs[s]
                qk2 = qk2p.tile([128, 8, S], F16, tag="qk2")
                o_t = a4.tile([128, KC, S], F16, tag="ot", bufs=2)
                ats = {}

                def st2(m):
                    ps = b1.tile([128, S], F32, tag="b1")
                    base = 0 if m < 4 else 4
                    for k in range(KC):
                        nc.tensor.matmul(ps[:], w2[:, k, m * 128:(m + 1) * 128],
                                         qkr[:, base + k, :],
                                         start=(k == 0), stop=(k == KC - 1))
                    nc.vector.tensor_tensor(qk2[:, m, :], ps[:], cq[:, m, :],
                                            ALU.add)

                def emit_scores(pair):
                    for hh in pair:
                        rows = slice(64 * (hh % 2), 64 * (hh % 2) + 64)
                        at = a4.tile([128, KC, S], F16, tag="at", bufs=4,
                                     name=f"at{hh}")
                        for c in range(KC):
                            scp = b1.tile([128, S], F32, tag="b1")
                            nc.tensor.matmul(
                                scp[:],
                                qk2[rows, 4 + hh // 2, c * 128:(c + 1) * 128],
                                qk2[rows, hh // 2, :], start=True, stop=True)
                            nc.scalar.activation(at[:, c, :], scp[:], AF.Exp,
                                                 scale=float(1.0 / np.sqrt(DH)))
                        ats[hh] = at

                # stage2 interleaved with the first two pairs' scores (each
                # pair j only needs qk2 chunks j and 4+j), so exp starts early;
                # later pairs' scores are emitted AFTER the previous pair's
                # A@V so their at-slot WAR never head-of-line-blocks the Act
                # queue in front of the denominator chain.
                pairs = [(0, 1), (2, 3), (4, 5), (6, 7)]
                st2(0); st2(4); st2(1); st2(5)
                emit_scores(pairs[0])
                st2(2); st2(6); st2(3); st2(7)
                emit_scores(pairs[1])
                for pi, pair in enumerate(pairs):
                    for hh in pair:
                        rows = slice(64 * (hh % 2), 64 * (hh % 2) + 64)
                        at = ats[hh]
                        ov = b2.tile([65, S], F32, tag="b2")
                        for c in range(KC):
                            nc.tensor.matmul(ov[:], vloc[:, c, hh, :], at[:, c, :],
                                             start=(c == 0), stop=(c == KC - 1))
                        # 1/denom = exp(-ln denom), broadcast via PE
                        lnd = scr.tile([1, S], F16, tag="lnd", bufs=2)
                        nc.scalar.activation(lnd[:], ov[64:65, :], AF.Ln)
                        rb = b2.tile([64, S], F32, tag="b2")
                        nc.tensor.matmul(rb[:], ones_row[:, 0:64], lnd[:],
                                         start=True, stop=True)
                        inv = invp.tile([64, S], F16, tag="inv")
                        nc.scalar.activation(inv[:], rb[:], AF.Exp, scale=-1.0)
                        nc.vector.tensor_tensor(o_t[rows, hh // 2, :],
                                                ov[0:64, :], inv[:], ALU.mult)
                    if pi + 2 < len(pairs):
                        emit_scores(pairs[pi + 2])
                # out-proj + residual
                for m in range(KC):
                    ps = b1.tile([128, S], F32, tag="b1")
                    for k in range(KC):
                        nc.tensor.matmul(ps[:], ow[:, k, m * 128:(m + 1) * 128],
                                         o_t[:, k, :], start=(k == 0),
                                         stop=(k == KC - 1))
                    nc.vector.scalar_tensor_tensor(
                        x[:, m, sl], ps[:], owb[:, m:m + 1], x[:, m, sl],
                        ALU.add, ALU.add)
                rc2[s] = ln_stats(x, s)
                h2s[s] = hp.tile([128, KC, S], F8, tag="h8", name=f"h2_{s}")
                ln_apply(rc2[s], x, s, h2s[s], scaled=True)

            # ---- phase B: FFN in fp8/DoubleRow (l2 streams in once w1 slot
            #      is released); biases ride in the PSUM via a rank-1 matmul.
            l2 = wpool.tile([128, FC, D], F8, tag="bigw")
            for kf in range(FC):
                nc.sync.dma_start(l2[:, kf, :], dram["el2T"][li][:, kf, :])
            for s in range(BL):
                sl = slice(s * S, (s + 1) * S)
                h2 = h2s[s]                               # h2 = HS*LN(x)
                f2 = [b1.tile([128, S], F32, tag="b1", name=f"f2_{_m}")
                      for _m in range(KC)]
                for m in range(KC):     # inject WOS*l2b into the accumulators
                    nc.tensor.matmul(f2[m][:], l2b[:, m * 128:(m + 1) * 128],
                                     ones_rowS[:], start=True, stop=False)
                for jf in range(FC // 2):
                    rt2 = rtp.tile([128, 2, S], F8, tag="rt8")
                    for i in range(2):
                        kf = 2 * jf + i
                        f1 = b2.tile([128, S], F32, tag="b2")
                        for c in range(2):
                            nc.tensor.matmul(
                                f1[:],
                                l1[:, 2 * c:2 * c + 2, kf * 128:(kf + 1) * 128],
                                h2[:, 2 * c:2 * c + 2, :],
                                start=(c == 0), stop=(c == 1), perf_mode=DR)
                        # rt = OS*relu(z + l1b):  f1 = WHS*z, bias = OS*l1b
                        nc.scalar.activation(rt2[:, i, :], f1[:], AF.Relu,
                                             scale=OS / WHS,
                                             bias=l1b[:, kf:kf + 1])
                    for m in range(KC):
                        nc.tensor.matmul(f2[m][:],
                                         l2[:, 2 * jf:2 * jf + 2, m * 128:(m + 1) * 128],
                                         rt2[:, :, :], start=False,
                                         stop=(jf == FC // 2 - 1), perf_mode=DR)
                for m in range(KC):
                    nc.vector.scalar_tensor_tensor(
                        x[:, m, sl], f2[m][:], 1.0 / WOS, x[:, m, sl],
                        ALU.mult, ALU.add)
                rc1[s] = ln_stats(x, s)   # next layer (or the final LN)
                if li + 1 < le:           # prefetch next layer's LN1 apply
                    hs_next[s] = hp.tile([128, KC, S], F16, tag="h",
                                         name=f"h_pre{s}")
                    ln_apply(rc1[s], x, s, hs_next[s])

        # ---------------- final encoder LN (in-place; affine folded) --------
        me = x
        for s in range(BL):
            ln_apply(rc1[s], x, s, x[:, :, s * S:(s + 1) * S])

        # ---------------- decoder layers ----------------
        for li in range(ld):
            dw = wpool.tile([128, KC, 3 * D], F16, tag="bigw")
            nc.sync.dma_start(dw[:], dram["dinT"][li])
            dwb = bpool.tile([128, 12], F32, tag="w1b")
            nc.sync.dma_start(dwb[:], dram["dinb"][li])
            dvbrow = bpool.tile([1, D], F16, tag="vbrow")
            nc.sync.dma_start(dvbrow[:], dram["dvb"][li])
            do = owpool.tile([128, KC, D], F16, tag="ow")
            nc.sync.dma_start(do[:], dram["dowT"][li])
            dob = bpool.tile([128, 4], F32, tag="owb")
            nc.sync.dma_start(dob[:], dram["dowb"][li])
            m1 = wpool.tile([128, KC, M], F16, tag="bigw")
            nc.sync.dma_start(m1[:], dram["dm1T"][li])
            m1b = bpool.tile([128, 16], F32, tag="l1b")
            nc.sync.dma_start(m1b[:], dram["dm1b"][li])
            m2b = bpool.tile([128, 4], F32, tag="l2b")
            nc.sync.dma_start(m2b[:], dram["dm2b"][li])

            vb_ps = b1.tile([128, D], F32, tag="b1")
            nc.tensor.matmul(vb_ps[:], ones_row[:], dvbrow[:], start=True, stop=True)
            vb_bc = vbp.tile([128, D], F16, tag="vb_bc")
            nc.scalar.activation(vb_bc[:], vb_ps[:], AF.Copy)

            # LN(p) -> q_ln ; Q projection (all samples at once, N=BL)
            q_ln = smalls.tile([128, KC, BL], F16, tag="q_ln")
            ln_small(p, BL, q_ln)
            qps = b1.tile([128, KC, BL], F32, tag="b1")
            for m in range(KC):
                for k in range(KC):
                    nc.tensor.matmul(qps[:, m, :],
                                     dw[:, k, m * 128:(m + 1) * 128],
                                     q_ln[:, k, :], start=(k == 0),
                                     stop=(k == KC - 1))
            q_sb = smalls.tile([128, KC, BL], F16, tag="q_sb")
            for m in range(KC):
                nc.scalar.activation(q_sb[:, m, :], qps[:, m, :], AF.Identity,
                                     bias=dwb[:, m:m + 1])
            o_d = smalls.tile([128, KC, BL], F16, tag="o_d")
            for s in range(BL):
                sl = slice(s * S, (s + 1) * S)
                # K (feature-major) and V' (token-major) over morph_enc
                k_sb = a4.tile([128, KC, S], F16, tag="at", bufs=4)
                for m in range(KC):
                    ps = b1.tile([128, S], F32, tag="b1")
                    for k in range(KC):
                        nc.tensor.matmul(
                            ps[:], dw[:, k, D + m * 128:D + (m + 1) * 128],
                            me[:, k, sl], start=(k == 0), stop=(k == KC - 1))
                    nc.scalar.activation(k_sb[:, m, :], ps[:], AF.Identity,
                                         bias=dwb[:, 4 + m:5 + m])
                vloc = vp.tile([128, KC, 8, 65], F16, tag="vloc")
                for t in range(KC):
                    nc.vector.tensor_copy(vloc[:, t, :, 64], ones8[:])
                for t in range(KC):
                    ps = b1.tile([128, S], F32, tag="b1")
                    for k in range(KC):
                        nc.tensor.matmul(
                            ps[:], me[:, k, s * S + t * 128:s * S + (t + 1) * 128],
                            dw[:, k, 2 * D:3 * D],
                            start=(k == 0), stop=(k == KC - 1))
                    nc.vector.tensor_tensor(
                        vloc[:, t, :, 0:64],
                        ps[:].rearrange("p (h d) -> p h d", h=H),
                        vb_bc[:].rearrange("p (h d) -> p h d", h=H), ALU.add)
                scp = b1.tile([128, KC, H], F32, tag="b1")
                for hh in range(H):
                    rows = slice(64 * (hh % 2), 64 * (hh % 2) + 64)
                    for c in range(KC):
                        nc.tensor.matmul(
                            scp[:, c, hh:hh + 1],
                            k_sb[rows, hh // 2, c * 128:(c + 1) * 128],
                            q_sb[rows, hh // 2, s:s + 1],
                            start=True, stop=True)
                at = smalls.tile([128, KC, H], F16, tag="at_d")
                nc.scalar.activation(at[:], scp[:], AF.Exp,
                                     scale=float(1.0 / np.sqrt(DH)))
                ov = b2.tile([65, H], F32, tag="b2")
                for hh in range(H):
                    for c in range(KC):
                        nc.tensor.matmul(ov[:, hh:hh + 1], vloc[:, c, hh, :],
                                         at[:, c, hh:hh + 1],
                                         start=(c == 0), stop=(c == KC - 1))
                # 1/denom = exp(-ln denom)
                lnd = scr.tile([1, H], F16, tag="lnd", bufs=2)
                nc.scalar.activation(lnd[:], ov[64:65, :], AF.Ln)
                rb = b2.tile([64, H], F32, tag="b2")
                nc.tensor.matmul(rb[:], ones_row[:, 0:64], lnd[:],
                                 start=True, stop=True)
                inv = invp.tile([64, H], F16, tag="inv_d")
                nc.scalar.activation(inv[:], rb[:], AF.Exp, scale=-1.0)
                for hh in range(H):
                    rows = slice(64 * (hh % 2), 64 * (hh % 2) + 64)
                    nc.vector.tensor_tensor(o_d[rows, hh // 2, s:s + 1],
                                            ov[0:64, hh:hh + 1],
                                            inv[:, hh:hh + 1], ALU.mult)
            # out-proj + residual into p
            ops = b1.tile([128, KC, BL], F32, tag="b1")
            for m in range(KC):
                for k in range(KC):
                    nc.tensor.matmul(ops[:, m, :],
                                     do[:, k, m * 128:(m + 1) * 128],
                                     o_d[:, k, :], start=(k == 0),
                                     stop=(k == KC - 1))
            for m in range(KC):
                nc.vector.scalar_tensor_tensor(
                    p[:, m, :], ops[:, m, :], dob[:, m:m + 1], p[:, m, :],
                    ALU.add, ALU.add)
            # FFN on p (m2 streams in chunked once dw releases its slot)
            m2 = wpool.tile([128, MC, D], F16, tag="bigw")
            for kf in range(MC):
                nc.sync.dma_start(m2[:, kf, :], dram["dm2T"][li][:, kf, :])
            h2d = smalls.tile([128, KC, BL], F16, tag="q_ln")
            ln_small(p, BL, h2d)
            mh = smalls.tile([128, MC, BL], F16, tag="mh")
            for mm_ in range(MC):
                ps = b1.tile([128, BL], F32, tag="b1")
                for k in range(KC):
                    nc.tensor.matmul(ps[:], m1[:, k, mm_ * 128:(mm_ + 1) * 128],
                                     h2d[:, k, :], start=(k == 0),
                                     stop=(k == KC - 1))
                nc.scalar.activation(mh[:, mm_, :], ps[:], AF.Relu,
                                     bias=m1b[:, mm_:mm_ + 1])
            m2ps = b1.tile([128, KC, BL], F32, tag="b1")
            for m in range(KC):
                for kf in range(MC):
                    nc.tensor.matmul(m2ps[:, m, :],
                                     m2[:, kf, m * 128:(m + 1) * 128],
                                     mh[:, kf, :], start=(kf == 0),
                                     stop=(kf == MC - 1))
            for m in range(KC):
                nc.vector.scalar_tensor_tensor(
                    p[:, m, :], m2ps[:, m, :], m2b[:, m:m + 1], p[:, m, :],
                    ALU.add, ALU.add)

        # ---------------- head ----------------
        hw = smalls.tile([128, KC], F16, tag="hw")
        hb = smalls.tile([1, 1], F32, tag="hb")
        nc.sync.dma_start(hw[:], dram["hwT"][:])
        nc.sync.dma_start(hb[:], dram["hb"][:])
        hg = smalls.tile([128, KC, BL], F16, tag="q_ln")
        ln_small(p, BL, hg)
        hps = b2.tile([1, BL], F32, tag="b2")
        for k in range(KC):
            nc.tensor.matmul(hps[:], hw[:, k:k + 1], hg[:, k, :],
                             start=(k == 0), stop=(k == KC - 1))
        y_sb = smalls.tile([1, BL], F32, tag="y_sb")
        nc.scalar.activation(y_sb[:], hps[:], AF.Sigmoid, bias=hb[:])
        nc.sync.dma_start(y_dram[:], y_sb[:])


# ----------------------------------------------------------------------------
# entry point
# ----------------------------------------------------------------------------

_NC_CACHE = {}


def kernel(**inputs):
    return _run(inputs, LE, LD)


def _run(inputs, le, ld, trace=False):
    w = prep_weights(inputs, le, ld)
    morph = np.asarray(inputs["morph"], np.float32)
    pose = np.asarray(inputs["pose"], np.float32)
    in_maps = []
    for c in range(NCORES):
        im = dict(w)
        mo = morph[c * BL:(c + 1) * BL]                 # [BL, S, 3]
        im["morphT"] = np.ascontiguousarray(
            mo.transpose(2, 0, 1).reshape(3, T)).astype(np.float16)
        im["poseT"] = np.ascontiguousarray(
            pose[c * BL:(c + 1) * BL].T).astype(np.float16)
        in_maps.append(im)

    if ("nc", le, ld) not in _NC_CACHE:
        _NC_CACHE[("nc", le, ld)] = build(le, ld)
    nc = _NC_CACHE[("nc", le, ld)]
    res = run_bass_kernel_spmd(nc, in_maps, core_ids=list(range(NCORES)),
                               trace=trace)
    out = np.zeros((B, 1), np.float32)
    for c in range(NCORES):
        out[c * BL:(c + 1) * BL, 0] = res.results[c]["y"][0]
    if trace:
        return out, res
    return out
